# revision 1
# baseline (speedup 1.0000x reference)
"""NequIP GNN message-passing kernel for 8 Trainium2 NeuronCores — v2.

Receiver-sharded graph parallelism (per sharding hint): host LPT-assigns the
8192 nodes to 64 (core, window) bins of 128, each core owns 8 windows = 1024
nodes + their in-edges, sorted by window, padded to 128-edge tiles.

v2 device pipeline (vs v1): bf16 edge pipeline end-to-end with fp32 PSUM
accumulation; per-edge-scalar stages batched across 8-tile groups via
stride-0 broadcast access patterns; P-products collapsed per-path (w operand
broadcast over l1 components); CG-FMA stage merged into diagonal runs with
immediate scalars; segment-sum via paired-kg bf16 matmuls; radial MLP with
fused Silu activations; elementwise work split across Vector/GpSimd/Scalar.
"""
import math
import numpy as np
import ml_dtypes

BF16 = ml_dtypes.bfloat16

# ---------------- model constants ----------------
N_NODES, N_EDGES = 8192, 131072
C, H, NRAD = 64, 64, 8
R_MAX, AVG_NN = 5.0, 16.0
NCORES, NPC = 8, 1024
NW = NPC // 128
F = 9 * C
LS = (0, 1, 2)
PATHS = [(l1, l2, l3) for l1 in LS for l2 in LS for l3 in LS
         if abs(l1 - l2) <= l3 <= l1 + l2]
LOFF = {0: 0, 1: 1, 2: 4}
J_OF_L2 = {0: [0], 1: [1, 2, 3], 2: [4, 5, 6, 7, 8]}
BP = 8                      # tiles per group (batched in op free dims)
GCOLP_L2 = 640              # padded gather row (bf16): 1280B % 256 == 0
GCOLP_L1 = 128              # 256B % 256 == 0


# ---------------- real Clebsch-Gordan coefficients ----------------
def _cg_scalar(j1, m1, j2, m2, j3, m3):
    f = math.factorial
    if m1 + m2 != m3:
        return 0.0
    pre = ((2*j3+1) * f(j1+j2-j3) * f(j1-j2+j3) * f(-j1+j2+j3)
           / f(j1+j2+j3+1)) ** 0.5
    pre *= (f(j1+m1)*f(j1-m1)*f(j2+m2)*f(j2-m2)*f(j3+m3)*f(j3-m3)) ** 0.5
    s = 0.0
    for k in range(max(0, j2-j3-m1, j1+m2-j3), min(j1+j2-j3, j1-m1, j2+m2)+1):
        s += (-1)**k / (f(k)*f(j1+j2-j3-k)*f(j1-m1-k)
                        * f(j2+m2-k)*f(j3-j2+m1+k)*f(j3-j1-m2+k))
    return pre * s


def _U_real(l):
    U = np.zeros((2*l+1, 2*l+1), dtype=complex)
    s2 = 2 ** -0.5
    for m in range(-l, l+1):
        if m > 0:
            U[m+l, m+l] = (-1)**m * s2
            U[m+l, -m+l] = s2
        elif m == 0:
            U[l, l] = 1.0
        else:
            U[m+l, m+l] = 1j*s2
            U[m+l, -m+l] = -1j*(-1)**(-m)*s2
    return U


def _real_cg(l1, l2, l3):
    Cc = np.zeros((2*l1+1, 2*l2+1, 2*l3+1))
    for i1, m1 in enumerate(range(-l1, l1+1)):
        for i2, m2 in enumerate(range(-l2, l2+1)):
            m3 = m1 + m2
            if abs(m3) <= l3:
                Cc[i1, i2, m3+l3] = _cg_scalar(l1, m1, l2, m2, l3, m3)
    U1, U2, U3 = _U_real(l1), _U_real(l2), _U_real(l3)
    W = np.einsum('ia,jb,kc,abc->ijk', U1.conj(), U2.conj(), U3,
                  Cc.astype(complex))
    W = W.real if np.linalg.norm(W.real) >= np.linalg.norm(W.imag) else W.imag
    W = W / np.linalg.norm(W) * (2*l3+1) ** 0.5
    return np.asarray(W, dtype=np.float64)


CGS = [_real_cg(*p) for p in PATHS]


def build_schedule_l2():
    """Static TP structure for layer 2, grouped by l2.

    Per group: nblk, pops=[(path, ig0, ni, z0)], jlist, runs=[(j,z0,k0,L,cg)].
    """
    scheds = []
    for l2 in (0, 1, 2):
        ps = [p for p in range(len(PATHS)) if PATHS[p][1] == l2]
        blocks, block_of, pops = [], {}, []
        for p in ps:
            l1 = PATHS[p][0]
            ni = 2 * l1 + 1
            pops.append((p, LOFF[l1], ni, len(blocks)))
            for i in range(ni):
                block_of[(p, i)] = len(blocks)
                blocks.append((p, i))
        tset = set()
        for p in ps:
            l1, _, l3 = PATHS[p]
            cg = CGS[p]
            for i in range(2*l1+1):
                for j in range(2*l2+1):
                    for k in range(2*l3+1):
                        v = cg[i, j, k]
                        if abs(v) > 1e-12:
                            tset.add((LOFF[l2]+j, block_of[(p, i)],
                                      LOFF[l3]+k, round(float(v), 9)))
        runs, consumed = [], set()
        for t in sorted(tset):
            if t in consumed:
                continue
            j, z, k, cgv = t
            L = 0
            while (j, z+L, k+L, cgv) in tset and (j, z+L, k+L, cgv) not in consumed:
                consumed.add((j, z+L, k+L, cgv))
                L += 1
            runs.append((j, z, k, L, cgv))
        scheds.append(dict(l2=l2, nblk=len(blocks), pops=pops,
                           jlist=J_OF_L2[l2], runs=runs))
    return scheds


SCHED_L2 = build_schedule_l2()


def build_schedule2():
    """cg folded into per-(j,z) host scalars; FMA mostly tensor adds.

    Per group: nblk, pops, zjq=[(j, qoff)], runs_add=[(j,z0,k0,L)],
    runs_stt=[(j,z,k,ratio)]. qoff = column offset of (j,*) block in the
    concatenated shq table [128, T, 179]. cgfold[qoff+z] = cg of the primary
    (smallest-k) triple of (j,z); 0 for unused (z,j).
    """
    scheds, qoff, cgcols = [], 0, []
    for gi, sch in enumerate(SCHED_L2):
        nblk = sch["nblk"]
        tset = set()
        for (j, z, k, L, cg) in sch["runs"]:
            for i in range(L):
                tset.add((j, z + i, k + i, cg))
        per_jz = {}
        for (j, z, k, cg) in tset:
            per_jz.setdefault((j, z), []).append((k, cg))
        primary, runs_stt = set(), []
        cgf = {}
        for (j, z), ks in per_jz.items():
            ks.sort()
            k0, cg0 = ks[0]
            primary.add((j, z, k0))
            cgf[(j, z)] = cg0
            for (k1, cg1) in ks[1:]:
                runs_stt.append((j, z, k1, cg1 / cg0))
        runs_add, consumed = [], set()
        for t in sorted(primary):
            if t in consumed:
                continue
            j, z, k = t
            L = 0
            while (j, z + L, k + L) in primary and (j, z + L, k + L) not in consumed:
                consumed.add((j, z + L, k + L))
                L += 1
            runs_add.append((j, z, k, L))
        zjq = []
        for j in sch["jlist"]:
            zjq.append((j, qoff))
            for z in range(nblk):
                cgcols.append((j, cgf.get((j, z), 0.0)))
            qoff += nblk
        scheds.append(dict(nblk=nblk, pops=sch["pops"], zjq=zjq,
                           runs_add=runs_add, runs_stt=runs_stt))
    return scheds, cgcols


SCHED2_L2, SHQ_COLS = build_schedule2()
NSHQ = len(SHQ_COLS)

# layer-1 per-k path and cg (x is scalar-only: paths (0,l,l), j == k)
L1_PATH_OF_K = [0] + [1]*3 + [2]*5
L1_CG_OF_K = []
for _k in range(9):
    _p = L1_PATH_OF_K[_k]
    _l = PATHS[_p][2]
    _m = _k - LOFF[_l]
    L1_CG_OF_K.append(float(CGS[_p][0, _m, _m]))


# ---------------- host-side graph preprocessing ----------------
def edge_geometry(positions, senders, receivers):
    rel = (positions[receivers] - positions[senders]) / R_MAX
    d = np.linalg.norm(rel, axis=-1)
    u = rel / np.maximum(d, 1e-6)[:, None]
    x, y, z = u[:, 0], u[:, 1], u[:, 2]
    sh = np.empty((len(d), 9), np.float32)
    sh[:, 0] = 1.0
    sh[:, 1] = np.sqrt(3.0) * y
    sh[:, 2] = np.sqrt(3.0) * z
    sh[:, 3] = np.sqrt(3.0) * x
    sh[:, 4] = np.sqrt(15.0) * x * y
    sh[:, 5] = np.sqrt(15.0) * y * z
    sh[:, 6] = np.sqrt(5.0) / 2 * (3 * z * z - 1.0)
    sh[:, 7] = np.sqrt(15.0) * x * z
    sh[:, 8] = np.sqrt(15.0) / 2 * (x * x - y * y)
    freqs = np.arange(1, NRAD + 1, dtype=np.float64)
    xr = np.clip(d, 1e-4, 1.0)[:, None].astype(np.float64)
    basis = (np.sqrt(2.0) * np.sin(freqs * np.pi * xr) / xr).astype(np.float32)
    cut = (0.5 * (np.cos(np.pi * np.clip(d, 0.0, 1.0)) + 1.0)).astype(np.float32)
    return (sh * cut[:, None]).astype(np.float32), basis


def partition_graph(receivers):
    import heapq
    deg = np.bincount(receivers, minlength=N_NODES)
    order = np.argsort(-deg, kind="stable")
    nbins = NCORES * NW
    load = np.zeros(nbins, np.int64)
    cnt = np.zeros(nbins, np.int64)
    owner = np.empty(N_NODES, np.int32)
    local = np.empty(N_NODES, np.int32)
    heap = [(0, b) for b in range(nbins)]
    heapq.heapify(heap)
    for n in order:
        while True:
            l, b = heapq.heappop(heap)
            if cnt[b] < 128:
                break
        owner[n] = b // NW
        local[n] = (b % NW) * 128 + cnt[b]
        cnt[b] += 1
        load[b] += deg[n]
        if cnt[b] < 128:
            heapq.heappush(heap, (int(load[b]), b))
    nodes_of = np.empty((NCORES, NPC), np.int64)
    for n in range(N_NODES):
        nodes_of[owner[n], local[n]] = n
    return owner, local, nodes_of, int(load.max())


def build_core_edges(receivers, owner, local, tpw):
    T = NW * tpw
    perm = np.full((NCORES, T * 128), -1, np.int64)
    for k in range(NCORES):
        eids = np.where(owner[receivers] == k)[0]
        lr = local[receivers[eids]]
        o = np.argsort(lr, kind="stable")
        eids, lr = eids[o], lr[o]
        w_of = lr // 128
        for w in range(NW):
            sel = eids[w_of == w]
            assert len(sel) <= tpw * 128, "tiles-per-window overflow"
            base = w * tpw * 128
            perm[k, base:base + len(sel)] = sel
    return perm


# ---------------- bass kernel builder ----------------
def build_layer_kernel(layer2, T, debug=False):
    import concourse.bass as bass
    import concourse.bacc as bacc
    import concourse.tile as tile
    import concourse.mybir as mybir
    from contextlib import ExitStack

    fp32 = mybir.dt.float32
    bf16 = mybir.dt.bfloat16
    AF = mybir.ActivationFunctionType
    ALU = mybir.AluOpType

    NPATH = 15 if layer2 else 3
    GCOLP = GCOLP_L2 if layer2 else GCOLP_L1
    W3COL = NPATH * C if layer2 else 576   # L1 w3 host-expanded to 9 kg blocks
    E_PAD = T * 128
    NG = T // BP
    assert T % BP == 0 and T % NW == 0
    tpw = T // NW

    import os as _os
    STAGE = int(_os.environ.get("KV2_STAGE", "5"))
    nc = bacc.Bacc("TRN2", target_bir_lowering=False)

    ftab = nc.dram_tensor("ftab", [N_NODES, GCOLP], bf16, kind="ExternalInput")
    sidx = nc.dram_tensor("sidx", [128, E_PAD // 16], mybir.dt.int16,
                          kind="ExternalInput")
    shp_d = nc.dram_tensor("shp", [128, T, 9], bf16, kind="ExternalInput")
    shq_d = nc.dram_tensor("shq", [128, T, NSHQ], bf16, kind="ExternalInput")
    basT_d = nc.dram_tensor("basisT", [NG, 8, BP * 128], bf16,
                            kind="ExternalInput")
    smat_d = nc.dram_tensor("smat", [128, T, 128], bf16, kind="ExternalInput")
    oldT_d = nc.dram_tensor("oldT", [64, 9 * NPC], fp32, kind="ExternalInput")
    w1_d = nc.dram_tensor("w1", [8, H], bf16, kind="ExternalInput")
    b1_d = nc.dram_tensor("b1", [H, 1], fp32, kind="ExternalInput")
    w2_d = nc.dram_tensor("w2", [H, H], bf16, kind="ExternalInput")
    b2_d = nc.dram_tensor("b2", [H, 1], fp32, kind="ExternalInput")
    w3_d = nc.dram_tensor("w3", [H, W3COL], bf16, kind="ExternalInput")
    lin_d = [nc.dram_tensor(f"lin{l}", [C, C], fp32, kind="ExternalInput")
             for l in range(3)]
    gw_d = [nc.dram_tensor(f"gw{l}", [C, C], fp32, kind="ExternalInput")
            for l in range(2)]
    gb_d = [nc.dram_tensor(f"gb{l}", [C, 1], fp32, kind="ExternalInput")
            for l in range(2)]
    newT_d = nc.dram_tensor("newT", [64, 9 * NPC], fp32,
                            kind="ExternalOutput")
    if debug:
        dbg_xg = nc.dram_tensor("dbg_xg", [128, BP, GCOLP], fp32,
                                kind="ExternalOutput")
        dbg_w = nc.dram_tensor("dbg_w", [128, BP, W3COL], fp32,
                               kind="ExternalOutput")
        dbg_msgs = nc.dram_tensor("dbg_msgs", [128, BP, F], fp32,
                                  kind="ExternalOutput")
        dbg_agg = nc.dram_tensor("dbg_agg", [128, NW, 640], fp32,
                                 kind="ExternalOutput")

    with tile.TileContext(nc) as tc, ExitStack() as ctx:
        consts = ctx.enter_context(tc.tile_pool(name="consts", bufs=1))
        idx_sb = consts.tile([128, E_PAD // 16], mybir.dt.int16)
        nc.sync.dma_start(idx_sb[:], sidx[:])
        w1_sb = consts.tile([8, H], bf16)
        nc.sync.dma_start(w1_sb[:], w1_d[:])
        b1_sb = consts.tile([H, 1], fp32)
        nc.sync.dma_start(b1_sb[:], b1_d[:])
        w2_sb = consts.tile([H, H], bf16)
        nc.sync.dma_start(w2_sb[:], w2_d[:])
        b2_sb = consts.tile([H, 1], fp32)
        nc.sync.dma_start(b2_sb[:], b2_d[:])
        w3_sb = consts.tile([H, W3COL], bf16)
        nc.sync.dma_start(w3_sb[:], w3_d[:])
        lin_sb = [consts.tile([C, C], fp32, name=f"lin{l}", tag=f"lin{l}")
                  for l in range(3)]
        for l in range(3):
            nc.sync.dma_start(lin_sb[l][:], lin_d[l][:])
        gw_sb = [consts.tile([C, C], fp32, name=f"gw{l}", tag=f"gw{l}")
                 for l in range(2)]
        gb_sb = [consts.tile([C, 1], fp32, name=f"gb{l}", tag=f"gb{l}")
                 for l in range(2)]
        for l in range(2):
            nc.sync.dma_start(gw_sb[l][:], gw_d[l][:])
            nc.sync.dma_start(gb_sb[l][:], gb_d[l][:])
        agg_sb = consts.tile([64, NW, 2, 5, 128], fp32)

        with ExitStack() as psctx:
            iop = psctx.enter_context(tc.tile_pool(name="iop", bufs=2))
            aggt = psctx.enter_context(tc.tile_pool(name="aggt", bufs=1))
            wp = psctx.enter_context(tc.tile_pool(name="wp", bufs=1))
            msgp = psctx.enter_context(tc.tile_pool(name="msgp", bufs=2))
            shbp = psctx.enter_context(tc.tile_pool(name="shbp", bufs=1))
            pp = psctx.enter_context(tc.tile_pool(name="pp", bufs=1))
            zjp = psctx.enter_context(tc.tile_pool(name="zjp", bufs=1))
            h_ps = psctx.enter_context(
                tc.tile_pool(name="h_ps", bufs=1, space="PSUM"))
            w_ps_pool = psctx.enter_context(
                tc.tile_pool(name="w_ps", bufs=1, space="PSUM"))
            agg_pool = psctx.enter_context(
                tc.tile_pool(name="agg_ps", bufs=2, space="PSUM"))

            agg_open = {}

            for g in range(NG):
                t0 = g * BP
                xg = iop.tile([128, BP, GCOLP], bf16, tag="xg")
                nc.gpsimd.dma_gather(
                    out_ap=xg[:],
                    in_ap=ftab[:],
                    idxs_ap=idx_sb[:, g * (BP * 8):(g + 1) * (BP * 8)],
                    num_idxs=BP * 128,
                    num_idxs_reg=BP * 128,
                    elem_size=GCOLP,
                )
                shp_t = iop.tile([128, BP, 9], bf16, tag="shp")
                nc.sync.dma_start(shp_t[:], shp_d[:, t0:t0 + BP, :])
                if layer2:
                    shq_t = iop.tile([128, BP, NSHQ], bf16, tag="shq")
                    nc.sync.dma_start(shq_t[:], shq_d[:, t0:t0 + BP, :])
                smt = iop.tile([128, BP, 128], bf16, tag="smt")
                nc.sync.dma_start(smt[:], smat_d[:, t0:t0 + BP, :])
                bas = iop.tile([8, BP * 128], bf16, tag="bas")
                nc.sync.dma_start(bas[:], basT_d[g, :, :])

                if STAGE < 2:
                    continue
                if not layer2:
                    # sh broadcast table for L1 (ScalarE, stride-0 copy)
                    shB = shbp.tile([128, BP, 9, C], bf16, tag="shB")
                    nc.scalar.copy(
                        shB[:],
                        shp_t[:, :, :, None].broadcast_to((128, BP, 9, C)))

                # ---- radial MLP (transposed; fused Silu) ----
                h1s = iop.tile([H, BP * 128], bf16, tag="h1s")
                h2s = iop.tile([H, BP * 128], bf16, tag="h2s")
                sg = iop.tile([H, BP * 128], bf16, tag="sg")
                for c0 in range(0, BP * 128, 512):
                    h1p = h_ps.tile([H, 512], fp32, tag="h")
                    nc.tensor.matmul(h1p[:], w1_sb[:],
                                     bas[:, c0:c0 + 512], start=True, stop=True)
                    nc.scalar.activation(sg[:, c0:c0 + 512], h1p[:],
                                         AF.Sigmoid, bias=b1_sb[:, 0:1])
                    nc.vector.scalar_tensor_tensor(
                        out=h1s[:, c0:c0 + 512], in0=h1p[:],
                        scalar=b1_sb[:, 0:1], in1=sg[:, c0:c0 + 512],
                        op0=ALU.add, op1=ALU.mult)
                for c0 in range(0, BP * 128, 512):
                    h2p = h_ps.tile([H, 512], fp32, tag="h")
                    nc.tensor.matmul(h2p[:], w2_sb[:],
                                     h1s[:, c0:c0 + 512], start=True, stop=True)
                    nc.scalar.activation(sg[:, c0:c0 + 512], h2p[:],
                                         AF.Sigmoid, bias=b2_sb[:, 0:1])
                    nc.vector.scalar_tensor_tensor(
                        out=h2s[:, c0:c0 + 512], in0=h2p[:],
                        scalar=b2_sb[:, 0:1], in1=sg[:, c0:c0 + 512],
                        op0=ALU.add, op1=ALU.mult)

                # ---- per-tile edge weights w = h2s_t.T @ W3 (bf16 out) ----
                w_sb = wp.tile([128, BP, W3COL], bf16, tag="wsb")
                for bt in range(BP):
                    w_ps = w_ps_pool.tile([128, W3COL], fp32, tag="wps")
                    for c0 in range(0, W3COL, 512):
                        c1 = min(c0 + 512, W3COL)
                        nc.tensor.matmul(w_ps[:, c0:c1],
                                         h2s[:, bt * 128:(bt + 1) * 128],
                                         w3_sb[:, c0:c1], start=True, stop=True)
                    nc.scalar.copy(w_sb[:, bt, :], w_ps[:, 0:W3COL])

                if STAGE < 3:
                    continue
                # ---- tensor product ----
                msgs = msgp.tile([128, BP, F], bf16, tag="msgs")
                if layer2:
                    _emit_tp_l2(nc, ALU, xg, w_sb, shq_t, msgs, pp, zjp)
                else:
                    _emit_tp_l1(nc, ALU, xg, w_sb, shB, msgs, pp)

                if debug and g == 0:
                    dxg = pp.tile([128, BP, GCOLP], fp32, tag="dxg")
                    nc.vector.tensor_copy(out=dxg[:], in_=xg[:])
                    nc.sync.dma_start(dbg_xg[:], dxg[:])
                    dw = pp.tile([128, BP, W3COL], fp32, tag="dw")
                    nc.vector.tensor_copy(out=dw[:], in_=w_sb[:])
                    nc.sync.dma_start(dbg_w[:], dw[:])
                    dmg = pp.tile([128, BP, F], fp32, tag="dmg")
                    nc.vector.tensor_copy(out=dmg[:], in_=msgs[:])
                    nc.sync.dma_start(dbg_msgs[:], dmg[:])

                if STAGE < 4:
                    continue
                # ---- segment sum: paired-kg bf16 matmuls, PSUM-accumulated ----
                for bt in range(BP):
                    t = t0 + bt
                    w, t_in_w = t // tpw, t % tpw
                    if w not in agg_open:
                        agg_open[w] = agg_pool.tile([128, 640], fp32,
                                                    name="aggps", tag="aggps")
                    ps = agg_open[w]
                    first, last = t_in_w == 0, t_in_w == tpw - 1
                    for pair in range(5):
                        c0 = pair * 128
                        cw = 128 if pair < 4 else 64
                        nc.tensor.matmul(
                            ps[0:cw, c0:c0 + 128],
                            msgs[:, bt, c0:c0 + cw],
                            smt[:, bt, :],
                            start=first and pair in (0, 4),
                            stop=last, skip_group_check=True)
                    if last:
                        aps = agg_open.pop(w)
                        afp = aggt.tile([128, 640], fp32, tag="afp")
                        nc.scalar.copy(afp[:, 0:512], aps[:, 0:512])
                        nc.scalar.copy(afp[0:64, 512:640], aps[0:64, 512:640])
                        nc.sync.dma_start(
                            agg_sb[:, w, 0, :, :],
                            afp[0:64, :].rearrange("p (q n) -> p q n", q=5))
                        nc.sync.dma_start(
                            agg_sb[:, w, 1, 0:4, :],
                            afp[64:128, 0:512].rearrange("p (q n) -> p q n",
                                                         q=4))



        # ---------------- per-window node update ----------------
        if STAGE < 5:
            with ExitStack() as upctx:
                upt = upctx.enter_context(tc.tile_pool(name="upt", bufs=2))
                for w in range(NW):
                    tmp = upt.tile([64, 9, 128], fp32, tag="pass")
                    nc.sync.dma_start(
                        tmp[:], oldT_d[:, :].rearrange(
                            "p (q n) -> p q n",
                            q=9)[:, :, w * 128:(w + 1) * 128])
                    nc.sync.dma_start(
                        newT_d[:, :].rearrange(
                            "p (q n) -> p q n",
                            q=9)[:, :, w * 128:(w + 1) * 128],
                        tmp[:])
            nc.compile()
            return nc
        with ExitStack() as upctx:
            y_pool = upctx.enter_context(
                tc.tile_pool(name="y_ps", bufs=2, space="PSUM"))
            g_pool = upctx.enter_context(
                tc.tile_pool(name="g_ps", bufs=2, space="PSUM"))
            upt = upctx.enter_context(tc.tile_pool(name="upt", bufs=2))
            for w in range(NW):
                oldw = upt.tile([64, 9, 128], fp32, tag="oldw")
                nc.sync.dma_start(
                    oldw[:], oldT_d[:, :].rearrange(
                        "p (q n) -> p q n", q=9)[:, :, w * 128:(w + 1) * 128])
                neww = upt.tile([64, 9, 128], fp32, tag="neww")
                y_ps = y_pool.tile([64, 9 * 128], fp32, tag="yps")
                for kg in range(9):
                    l = 0 if kg == 0 else (1 if kg <= 3 else 2)
                    nc.tensor.matmul(
                        y_ps[:, kg * 128:(kg + 1) * 128],
                        lin_sb[l][:],
                        agg_sb[:, w, kg % 2, kg // 2, :],
                        start=kg in (0, 4, 8), stop=True,
                        skip_group_check=True)
                y0g = upt.tile([C, 128], fp32, tag="y0g")
                nc.scalar.activation(y0g[:], y_ps[:, 0:128], AF.Sigmoid)
                y0s = upt.tile([C, 128], fp32, tag="y0s")
                nc.vector.tensor_tensor(out=y0s[:], in0=y_ps[:, 0:128],
                                        in1=y0g[:], op=ALU.mult)
                nc.vector.tensor_tensor(out=neww[:, 0, :], in0=y0s[:],
                                        in1=oldw[:, 0, :], op=ALU.add)
                g_ps = g_pool.tile([C, 2, 128], fp32, tag="gps")
                for l in (1, 2):
                    nc.tensor.matmul(g_ps[:, l - 1, :], gw_sb[l - 1][:],
                                     neww[:, 0, :], start=(l == 1), stop=True,
                                     skip_group_check=True)
                gts = upt.tile([C, 2, 128], fp32, tag="gts")
                for l in (1, 2):
                    nc.scalar.activation(gts[:, l - 1, :], g_ps[:, l - 1, :],
                                         AF.Sigmoid, bias=gb_sb[l - 1][:, 0:1])
                gy = upt.tile([C, 8, 128], fp32, tag="gy")
                nc.vector.tensor_tensor(
                    out=gy[:, 0:3, :],
                    in0=y_ps[:].rearrange("p (q n) -> p q n", q=9)[:, 1:4, :],
                    in1=gts[:, 0:1, :].broadcast_to((C, 3, 128)),
                    op=ALU.mult)
                nc.vector.tensor_tensor(
                    out=gy[:, 3:8, :],
                    in0=y_ps[:].rearrange("p (q n) -> p q n", q=9)[:, 4:9, :],
                    in1=gts[:, 1:2, :].broadcast_to((C, 5, 128)),
                    op=ALU.mult)
                nc.vector.tensor_tensor(out=neww[:, 1:9, :], in0=gy[:],
                                        in1=oldw[:, 1:9, :], op=ALU.add)
                nc.sync.dma_start(
                    newT_d[:, :].rearrange("p (q n) -> p q n",
                                           q=9)[:, :, w * 128:(w + 1) * 128],
                    neww[:])

    nc.compile()
    return nc


def _emit_tp_l2(nc, ALU, xg, w_sb, shq_t, msgs, pp, zjp):
    """P products, cg-folded zjQ scalings, FMA as adds (+8 ratio stts).

    All on DVE: single in-order queue, no cross-engine ping-pong; GpSimd
    has ~9us fixed cost per elementwise op and ScalarE cannot multiply
    two tensors, so the TP bulk lives here.
    """
    import concourse.mybir as mybir
    bf16 = mybir.dt.bfloat16

    GP_JS = (8,)        # l2=2 zjQ op offloaded to GpSimd (big ops only)
    P0 = pp.tile([128, BP, SCHED2_L2[0]["nblk"] * C], bf16, name="P0", tag="P0")
    P12 = pp.tile([128, BP, SCHED2_L2[2]["nblk"] * C], bf16,
                  name="P12", tag="P12")
    zj12 = zjp.tile([128, BP, SCHED2_L2[2]["nblk"] * C], bf16,
                    name="zj12", tag="zj12")
    zjGbuf = [zjp.tile([128, BP, SCHED2_L2[2]["nblk"] * C], bf16,
                       name=f"zjG{i}", tag=f"zjG{i}") for i in range(1)]
    zjG = {j: zjGbuf[0] for j in GP_JS}

    def emit_zjq(eng, dst, P, nblk, qoff):
        eng.tensor_tensor(
            out=dst[:].rearrange("p b (n c) -> p b n c",
                                 c=C)[:, :, 0:nblk, :],
            in0=P[:].rearrange("p b (n c) -> p b n c",
                               c=C)[:, :, 0:nblk, :],
            in1=shq_t[:, :, qoff:qoff + nblk, None].broadcast_to(
                (128, BP, nblk, C)),
            op=ALU.mult)

    def emit_fma(sch, j, zj):
        for (jj, z0, k0, L) in sch["runs_add"]:
            if jj != j:
                continue
            nc.vector.tensor_tensor(
                out=msgs[:, :, k0 * C:(k0 + L) * C],
                in0=zj[:, :, z0 * C:(z0 + L) * C],
                in1=msgs[:, :, k0 * C:(k0 + L) * C],
                op=ALU.add)
        for (jj, z, k, ratio) in sch["runs_stt"]:
            if jj != j:
                continue
            nc.vector.scalar_tensor_tensor(
                out=msgs[:, :, k * C:(k + 1) * C],
                in0=zj[:, :, z * C:(z + 1) * C],
                scalar=float(ratio),
                in1=msgs[:, :, k * C:(k + 1) * C],
                op0=ALU.mult, op1=ALU.add)

    for gi, sch in enumerate(SCHED2_L2):
        nblk = sch["nblk"]
        P = P0 if gi == 0 else P12
        for (p, ig0, ni, z0) in sch["pops"]:
            nc.vector.tensor_tensor(
                out=P[:].rearrange("p b (n c) -> p b n c",
                                   c=C)[:, :, z0:z0 + ni, :],
                in0=xg[:].rearrange("p b (n c) -> p b n c",
                                    c=C)[:, :, ig0:ig0 + ni, :],
                in1=w_sb[:, :, None, p * C:(p + 1) * C].broadcast_to(
                    (128, BP, ni, C)),
                op=ALU.mult)
        if gi > 0:
            # kick GpSimd's share first so it overlaps DVE's other j's
            for (j, qoff) in sch["zjq"]:
                if j in GP_JS:
                    emit_zjq(nc.gpsimd, zjG[j], P, nblk, qoff)
        for (j, qoff) in sch["zjq"]:
            if gi > 0 and j in GP_JS:
                continue
            dst = msgs if gi == 0 else zj12
            emit_zjq(nc.vector, dst, P, nblk, qoff)
            if gi == 0:
                continue
            emit_fma(sch, j, zj12)
        if gi > 0:
            for j in GP_JS:
                if any(jj == j for (jj, _) in sch["zjq"]):
                    emit_fma(sch, j, zjG[j])


def _emit_tp_l1(nc, ALU, xg, w_sb, shB, msgs, pp):
    """msgs[k] = x * w'_k * sh_k ; w' host-expanded+cg-folded to 9 kg blocks."""
    import concourse.mybir as mybir
    bf16 = mybir.dt.bfloat16
    tmp = pp.tile([128, BP, 9, C], bf16, name="l1tmp", tag="l1tmp")
    nc.vector.tensor_tensor(
        out=tmp[:],
        in0=w_sb[:].rearrange("p b (n c) -> p b n c", c=C),
        in1=shB[:],
        op=ALU.mult)
    nc.vector.tensor_tensor(
        out=msgs[:].rearrange("p b (n c) -> p b n c", c=C),
        in0=tmp[:],
        in1=xg[:, :, None, 0:C].broadcast_to((128, BP, 9, C)),
        op=ALU.mult)


# ---------------- host orchestration ----------------
def _chunked_T(feats_own):
    """[NPC, 576] -> kg-blocked transposed [64, 9*NPC]."""
    out = np.empty((64, 9 * NPC), np.float32)
    for kg in range(9):
        out[:, kg * NPC:(kg + 1) * NPC] = feats_own[:, kg * 64:(kg + 1) * 64].T
    return out


def _unchunk_T(newT):
    """[64, 9*NPC] -> [NPC, 576]."""
    out = np.empty((NPC, 576), np.float32)
    for kg in range(9):
        out[:, kg * 64:(kg + 1) * 64] = newT[:, kg * NPC:(kg + 1) * NPC].T
    return out


_CACHE = {}


def _prep(positions, senders, receivers):
    key = (senders.tobytes(), receivers.tobytes(), positions.tobytes())
    if _CACHE.get("key") == key:
        return _CACHE["val"]
    sh_eff, basis = edge_geometry(positions, senders, receivers)
    owner, local, nodes_of, _ = partition_graph(receivers)
    deg_bin = np.zeros(NCORES * NW, np.int64)
    np.add.at(deg_bin, owner[receivers] * NW + local[receivers] // 128, 1)
    tpw = (int(deg_bin.max()) + 127) // 128
    T = NW * tpw
    assert T % BP == 0
    perm = build_core_edges(receivers, owner, local, tpw)

    valid = perm >= 0
    eg = np.where(valid, perm, 0)
    snd = np.where(valid, senders[eg], 0).astype(np.int16)      # [NC, T*128]
    shp_e = sh_eff[eg] * valid[..., None]                        # [NC, T*128, 9]
    bas_e = basis[eg] * valid[..., None]                         # [NC, T*128, 8]
    lr = np.where(valid, local[receivers[eg]], 0)

    NG = T // BP
    inv = np.float32(1.0 / np.sqrt(AVG_NN))
    sidx = np.empty((NCORES, 128, T * 128 // 16), np.int16)
    shp_h = np.empty((NCORES, 128, T, 9), BF16)
    shq_h = np.empty((NCORES, 128, T, NSHQ), BF16)
    jcols = np.array([j for (j, cg) in SHQ_COLS])
    cgv = np.array([cg for (j, cg) in SHQ_COLS], np.float32)
    bas_h = np.empty((NCORES, NG, 8, BP * 128), BF16)
    smat_h = np.zeros((NCORES, 128, T, 128), BF16)
    for k in range(NCORES):
        s = snd[k].reshape(T * 8, 16)
        sidx[k] = np.tile(s.T, (8, 1))
        shp_f = shp_e[k].reshape(T, 128, 9).transpose(1, 0, 2)
        shp_h[k] = shp_f.astype(BF16)
        shq_h[k] = (shp_f[:, :, jcols] * cgv[None, None, :]).astype(BF16)
        bas_h[k] = bas_e[k].reshape(NG, BP * 128, 8).transpose(0, 2, 1).astype(BF16)
        v = valid[k]
        e_slots = np.arange(T * 128)
        p_, t_ = e_slots % 128, e_slots // 128
        cols = lr[k] - (t_ // tpw) * 128
        ok = v & (cols >= 0) & (cols < 128)
        sm = np.zeros((128, T, 128), np.float32)
        sm[p_[ok], t_[ok], cols[ok]] = inv
        smat_h[k] = sm.astype(BF16)
    val = dict(T=T, NG=NG, tpw=tpw, nodes_of=nodes_of, sidx=sidx,
               shp_h=shp_h, bas_h=bas_h, smat_h=smat_h, shq_h=shq_h)
    _CACHE["key"], _CACHE["val"] = key, val
    return val


EXEC_NS = []


def _run_layer(nc, pre, ftab, oldT_by_core, lw):
    import os
    from concourse.bass_utils import run_bass_kernel_spmd
    in_maps = []
    for k in range(NCORES):
        m = dict(ftab=ftab,
                 sidx=pre["sidx"][k],
                 shp=pre["shp_h"][k],
                 shq=pre["shq_h"][k],
                 basisT=pre["bas_h"][k],
                 smat=pre["smat_h"][k],
                 oldT=oldT_by_core[k],
                 w1=lw["w1"], b1=lw["b1"], w2=lw["w2"], b2=lw["b2"],
                 w3=lw["w3"], lin0=lw["lin"][0], lin1=lw["lin"][1],
                 lin2=lw["lin"][2], gw0=lw["gw"][0], gw1=lw["gw"][1],
                 gb0=lw["gb"][0], gb1=lw["gb"][1])
        in_maps.append(m)
    trace = bool(os.environ.get("KERNEL_TRACE"))
    res = run_bass_kernel_spmd(nc, in_maps, list(range(NCORES)), trace=trace,
                               trace_cores=list(range(NCORES)) if trace else None)
    if trace and res.exec_time_ns is not None:
        EXEC_NS.append(res.exec_time_ns)
    return [res.results[k]["newT"] for k in range(NCORES)]


def _layer_weights(inputs, i, layer2):
    f32 = np.float32
    w3 = np.ascontiguousarray(inputs["mlp_w3"][i], f32)
    if layer2:
        w3p = w3.astype(BF16)
    else:
        w3p = np.empty((H, 576), BF16)
        for kg in range(9):
            p = L1_PATH_OF_K[kg]
            w3p[:, kg * C:(kg + 1) * C] = (
                w3[:, p * C:(p + 1) * C] * L1_CG_OF_K[kg]).astype(BF16)
    return dict(
        w1=np.ascontiguousarray(inputs["mlp_w1"][i], f32).astype(BF16),
        b1=np.ascontiguousarray(inputs["mlp_b1"][i], f32).reshape(H, 1),
        w2=np.ascontiguousarray(inputs["mlp_w2"][i], f32).astype(BF16),
        b2=np.ascontiguousarray(inputs["mlp_b2"][i], f32).reshape(H, 1),
        w3=w3p,
        lin=[np.ascontiguousarray(inputs["lin_self"][i, l], f32)
             for l in range(3)],
        gw=[np.ascontiguousarray(inputs["gate_w"][i, l], f32)
            for l in range(2)],
        gb=[np.ascontiguousarray(inputs["gate_b"][i, l], f32).reshape(C, 1)
            for l in range(2)],
    )


_KERNEL_CACHE = {}


def _get_kernels(T):
    if T not in _KERNEL_CACHE:
        _KERNEL_CACHE[T] = (build_layer_kernel(False, T),
                            build_layer_kernel(True, T))
    return _KERNEL_CACHE[T]


def _pack_ftab(table, ncols):
    out = np.zeros((N_NODES, ncols), BF16)
    used = min(ncols, table.shape[1])
    out[:, :used] = table[:, :used].astype(BF16)
    return out


def kernel(**inputs):
    positions = np.asarray(inputs["positions"], np.float32)
    species = np.asarray(inputs["species"]).astype(np.int64)
    senders = np.asarray(inputs["senders"]).astype(np.int64)
    receivers = np.asarray(inputs["receivers"]).astype(np.int64)

    pre = _prep(positions, senders, receivers)
    T = pre["T"]
    nc1, nc2 = _get_kernels(T)
    nodes_of = pre["nodes_of"]

    # initial features: x0 from species embedding (host; tiny)
    x0 = (np.asarray(inputs["embed"], np.float32)[species]
          @ np.asarray(inputs["w_proj"], np.float32))          # [N, 64]
    table = np.zeros((N_NODES, F), np.float32)
    table[:, 0:C] = x0

    # ---- layer 1 ----
    oldT = [_chunked_T(table[nodes_of[k]]) for k in range(NCORES)]
    lw = _layer_weights(inputs, 0, False)
    newT = _run_layer(nc1, pre, _pack_ftab(table, GCOLP_L1), oldT, lw)

    table2 = np.empty((N_NODES, F), np.float32)
    for k in range(NCORES):
        table2[nodes_of[k]] = _unchunk_T(newT[k])

    # ---- layer 2 ----
    lw = _layer_weights(inputs, 1, True)
    newT2 = _run_layer(nc2, pre, _pack_ftab(table2, GCOLP_L2), newT, lw)

    table3 = np.empty((N_NODES, F), np.float32)
    for k in range(NCORES):
        table3[nodes_of[k]] = _unchunk_T(newT2[k])

    # ---- output: reorder component-major -> reference layout + alpha ----
    t3 = table3.reshape(N_NODES, 9, C)
    out = np.empty((N_NODES, F), np.float32)
    out[:, 0:64] = t3[:, 0]
    out[:, 64:256] = (0.5 * t3[:, 1:4]).transpose(0, 2, 1).reshape(N_NODES, 192)
    out[:, 256:576] = (0.25 * t3[:, 4:9]).transpose(0, 2, 1).reshape(N_NODES, 320)
    return out



# revision 19
# speedup vs baseline: 1.1221x; 1.1221x over previous
"""NequIP GNN message-passing kernel for 8 Trainium2 NeuronCores — v2.

Receiver-sharded graph parallelism (per sharding hint): host LPT-assigns the
8192 nodes to 64 (core, window) bins of 128, each core owns 8 windows = 1024
nodes + their in-edges, sorted by window, padded to 128-edge tiles.

v2 device pipeline (vs v1): bf16 edge pipeline end-to-end with fp32 PSUM
accumulation; per-edge-scalar stages batched across 8-tile groups via
stride-0 broadcast access patterns; P-products collapsed per-path (w operand
broadcast over l1 components); CG-FMA stage merged into diagonal runs with
immediate scalars; segment-sum via paired-kg bf16 matmuls; radial MLP with
fused Silu activations; elementwise work split across Vector/GpSimd/Scalar.
"""
import math
import numpy as np
import ml_dtypes

BF16 = ml_dtypes.bfloat16

# ---------------- model constants ----------------
N_NODES, N_EDGES = 8192, 131072
C, H, NRAD = 64, 64, 8
R_MAX, AVG_NN = 5.0, 16.0
NCORES, NPC = 8, 1024
NW = NPC // 128
F = 9 * C
LS = (0, 1, 2)
PATHS = [(l1, l2, l3) for l1 in LS for l2 in LS for l3 in LS
         if abs(l1 - l2) <= l3 <= l1 + l2]
LOFF = {0: 0, 1: 1, 2: 4}
J_OF_L2 = {0: [0], 1: [1, 2, 3], 2: [4, 5, 6, 7, 8]}
BP = 8                      # tiles per group (batched in op free dims)
GCOLP_L2 = 640              # padded gather row (bf16): 1280B % 256 == 0
GCOLP_L1 = 128              # 256B % 256 == 0


# ---------------- real Clebsch-Gordan coefficients ----------------
def _cg_scalar(j1, m1, j2, m2, j3, m3):
    f = math.factorial
    if m1 + m2 != m3:
        return 0.0
    pre = ((2*j3+1) * f(j1+j2-j3) * f(j1-j2+j3) * f(-j1+j2+j3)
           / f(j1+j2+j3+1)) ** 0.5
    pre *= (f(j1+m1)*f(j1-m1)*f(j2+m2)*f(j2-m2)*f(j3+m3)*f(j3-m3)) ** 0.5
    s = 0.0
    for k in range(max(0, j2-j3-m1, j1+m2-j3), min(j1+j2-j3, j1-m1, j2+m2)+1):
        s += (-1)**k / (f(k)*f(j1+j2-j3-k)*f(j1-m1-k)
                        * f(j2+m2-k)*f(j3-j2+m1+k)*f(j3-j1-m2+k))
    return pre * s


def _U_real(l):
    U = np.zeros((2*l+1, 2*l+1), dtype=complex)
    s2 = 2 ** -0.5
    for m in range(-l, l+1):
        if m > 0:
            U[m+l, m+l] = (-1)**m * s2
            U[m+l, -m+l] = s2
        elif m == 0:
            U[l, l] = 1.0
        else:
            U[m+l, m+l] = 1j*s2
            U[m+l, -m+l] = -1j*(-1)**(-m)*s2
    return U


def _real_cg(l1, l2, l3):
    Cc = np.zeros((2*l1+1, 2*l2+1, 2*l3+1))
    for i1, m1 in enumerate(range(-l1, l1+1)):
        for i2, m2 in enumerate(range(-l2, l2+1)):
            m3 = m1 + m2
            if abs(m3) <= l3:
                Cc[i1, i2, m3+l3] = _cg_scalar(l1, m1, l2, m2, l3, m3)
    U1, U2, U3 = _U_real(l1), _U_real(l2), _U_real(l3)
    W = np.einsum('ia,jb,kc,abc->ijk', U1.conj(), U2.conj(), U3,
                  Cc.astype(complex))
    W = W.real if np.linalg.norm(W.real) >= np.linalg.norm(W.imag) else W.imag
    W = W / np.linalg.norm(W) * (2*l3+1) ** 0.5
    return np.asarray(W, dtype=np.float64)


CGS = [_real_cg(*p) for p in PATHS]


def build_schedule_l2():
    """Static TP structure for layer 2, grouped by l2.

    Per group: nblk, pops=[(path, ig0, ni, z0)], jlist, runs=[(j,z0,k0,L,cg)].
    """
    scheds = []
    for l2 in (0, 1, 2):
        ps = [p for p in range(len(PATHS)) if PATHS[p][1] == l2]
        blocks, block_of, pops = [], {}, []
        for p in ps:
            l1 = PATHS[p][0]
            ni = 2 * l1 + 1
            pops.append((p, LOFF[l1], ni, len(blocks)))
            for i in range(ni):
                block_of[(p, i)] = len(blocks)
                blocks.append((p, i))
        tset = set()
        for p in ps:
            l1, _, l3 = PATHS[p]
            cg = CGS[p]
            for i in range(2*l1+1):
                for j in range(2*l2+1):
                    for k in range(2*l3+1):
                        v = cg[i, j, k]
                        if abs(v) > 1e-12:
                            tset.add((LOFF[l2]+j, block_of[(p, i)],
                                      LOFF[l3]+k, round(float(v), 9)))
        runs, consumed = [], set()
        for t in sorted(tset):
            if t in consumed:
                continue
            j, z, k, cgv = t
            L = 0
            while (j, z+L, k+L, cgv) in tset and (j, z+L, k+L, cgv) not in consumed:
                consumed.add((j, z+L, k+L, cgv))
                L += 1
            runs.append((j, z, k, L, cgv))
        scheds.append(dict(l2=l2, nblk=len(blocks), pops=pops,
                           jlist=J_OF_L2[l2], runs=runs))
    return scheds


SCHED_L2 = build_schedule_l2()


def build_schedule2():
    """cg folded into per-(j,z) host scalars; FMA mostly tensor adds.

    Per group: nblk, pops, zjq=[(j, qoff)], runs_add=[(j,z0,k0,L)],
    runs_stt=[(j,z,k,ratio)]. qoff = column offset of (j,*) block in the
    concatenated shq table [128, T, 179]. cgfold[qoff+z] = cg of the primary
    (smallest-k) triple of (j,z); 0 for unused (z,j).
    """
    scheds, qoff, cgcols = [], 0, []
    for gi, sch in enumerate(SCHED_L2):
        nblk = sch["nblk"]
        tset = set()
        for (j, z, k, L, cg) in sch["runs"]:
            for i in range(L):
                tset.add((j, z + i, k + i, cg))
        per_jz = {}
        for (j, z, k, cg) in tset:
            per_jz.setdefault((j, z), []).append((k, cg))
        primary, runs_stt = set(), []
        cgf = {}
        for (j, z), ks in per_jz.items():
            ks.sort()
            k0, cg0 = ks[0]
            primary.add((j, z, k0))
            cgf[(j, z)] = cg0
            for (k1, cg1) in ks[1:]:
                runs_stt.append((j, z, k1, cg1 / cg0))
        runs_add, consumed = [], set()
        for t in sorted(primary):
            if t in consumed:
                continue
            j, z, k = t
            L = 0
            while (j, z + L, k + L) in primary and (j, z + L, k + L) not in consumed:
                consumed.add((j, z + L, k + L))
                L += 1
            runs_add.append((j, z, k, L))
        zjq = []
        for j in sch["jlist"]:
            # dense full-range segment == baseline instruction stream
            zjq.append((j, qoff, [(0, nblk)]))
            for z in range(nblk):
                cgcols.append((j, cgf.get((j, z), 0.0)))
            qoff += nblk
        scheds.append(dict(nblk=nblk, pops=sch["pops"], zjq=zjq,
                           runs_add=runs_add, runs_stt=runs_stt))
    return scheds, cgcols


SCHED2_L2, SHQ_COLS = build_schedule2()
NSHQ = len(SHQ_COLS)

# layer-1 per-k path and cg (x is scalar-only: paths (0,l,l), j == k)
L1_PATH_OF_K = [0] + [1]*3 + [2]*5
L1_CG_OF_K = []
for _k in range(9):
    _p = L1_PATH_OF_K[_k]
    _l = PATHS[_p][2]
    _m = _k - LOFF[_l]
    L1_CG_OF_K.append(float(CGS[_p][0, _m, _m]))


# ---------------- host-side graph preprocessing ----------------
def edge_geometry(positions, senders, receivers):
    rel = (positions[receivers] - positions[senders]) / R_MAX
    d = np.linalg.norm(rel, axis=-1)
    u = rel / np.maximum(d, 1e-6)[:, None]
    x, y, z = u[:, 0], u[:, 1], u[:, 2]
    sh = np.empty((len(d), 9), np.float32)
    sh[:, 0] = 1.0
    sh[:, 1] = np.sqrt(3.0) * y
    sh[:, 2] = np.sqrt(3.0) * z
    sh[:, 3] = np.sqrt(3.0) * x
    sh[:, 4] = np.sqrt(15.0) * x * y
    sh[:, 5] = np.sqrt(15.0) * y * z
    sh[:, 6] = np.sqrt(5.0) / 2 * (3 * z * z - 1.0)
    sh[:, 7] = np.sqrt(15.0) * x * z
    sh[:, 8] = np.sqrt(15.0) / 2 * (x * x - y * y)
    freqs = np.arange(1, NRAD + 1, dtype=np.float64)
    xr = np.clip(d, 1e-4, 1.0)[:, None].astype(np.float64)
    basis = (np.sqrt(2.0) * np.sin(freqs * np.pi * xr) / xr).astype(np.float32)
    cut = (0.5 * (np.cos(np.pi * np.clip(d, 0.0, 1.0)) + 1.0)).astype(np.float32)
    return (sh * cut[:, None]).astype(np.float32), basis


def partition_graph(receivers):
    import heapq
    deg = np.bincount(receivers, minlength=N_NODES)
    order = np.argsort(-deg, kind="stable")
    nbins = NCORES * NW
    load = np.zeros(nbins, np.int64)
    cnt = np.zeros(nbins, np.int64)
    owner = np.empty(N_NODES, np.int32)
    local = np.empty(N_NODES, np.int32)
    heap = [(0, b) for b in range(nbins)]
    heapq.heapify(heap)
    for n in order:
        while True:
            l, b = heapq.heappop(heap)
            if cnt[b] < 128:
                break
        owner[n] = b // NW
        local[n] = (b % NW) * 128 + cnt[b]
        cnt[b] += 1
        load[b] += deg[n]
        if cnt[b] < 128:
            heapq.heappush(heap, (int(load[b]), b))
    nodes_of = np.empty((NCORES, NPC), np.int64)
    for n in range(N_NODES):
        nodes_of[owner[n], local[n]] = n
    return owner, local, nodes_of, int(load.max())


def build_core_edges(receivers, owner, local, tpw):
    T = NW * tpw
    perm = np.full((NCORES, T * 128), -1, np.int64)
    for k in range(NCORES):
        eids = np.where(owner[receivers] == k)[0]
        lr = local[receivers[eids]]
        o = np.argsort(lr, kind="stable")
        eids, lr = eids[o], lr[o]
        w_of = lr // 128
        for w in range(NW):
            sel = eids[w_of == w]
            assert len(sel) <= tpw * 128, "tiles-per-window overflow"
            base = w * tpw * 128
            perm[k, base:base + len(sel)] = sel
    return perm


# ---------------- bass kernel builder ----------------
def build_layer_kernel(layer2, T, debug=False):
    import concourse.bass as bass
    import concourse.bacc as bacc
    import concourse.tile as tile
    import concourse.mybir as mybir
    from contextlib import ExitStack

    fp32 = mybir.dt.float32
    bf16 = mybir.dt.bfloat16
    AF = mybir.ActivationFunctionType
    ALU = mybir.AluOpType

    NPATH = 15 if layer2 else 3
    GCOLP = GCOLP_L2 if layer2 else GCOLP_L1
    W3COL = NPATH * C if layer2 else 576   # L1 w3 host-expanded to 9 kg blocks
    E_PAD = T * 128
    NG = T // BP
    assert T % BP == 0 and T % NW == 0
    tpw = T // NW

    import os as _os
    STAGE = int(_os.environ.get("KV2_STAGE", "5"))
    nc = bacc.Bacc("TRN2", target_bir_lowering=False)

    ftab = nc.dram_tensor("ftab", [N_NODES, GCOLP], bf16, kind="ExternalInput")
    sidx = nc.dram_tensor("sidx", [128, E_PAD // 16], mybir.dt.int16,
                          kind="ExternalInput")
    shp_d = nc.dram_tensor("shp", [128, T, 9], bf16, kind="ExternalInput")
    shq_d = nc.dram_tensor("shq", [128, T, NSHQ], bf16, kind="ExternalInput")
    basT_d = nc.dram_tensor("basisT", [NG, 8, BP * 128], bf16,
                            kind="ExternalInput")
    smat_d = nc.dram_tensor("smat", [128, T, 128], bf16, kind="ExternalInput")
    oldT_d = nc.dram_tensor("oldT", [64, 9 * NPC], fp32, kind="ExternalInput")
    w1_d = nc.dram_tensor("w1", [8, H], bf16, kind="ExternalInput")
    b1_d = nc.dram_tensor("b1", [H, 1], fp32, kind="ExternalInput")
    w2_d = nc.dram_tensor("w2", [H, H], bf16, kind="ExternalInput")
    b2_d = nc.dram_tensor("b2", [H, 1], fp32, kind="ExternalInput")
    w3_d = nc.dram_tensor("w3", [H, W3COL], bf16, kind="ExternalInput")
    lin_d = [nc.dram_tensor(f"lin{l}", [C, C], fp32, kind="ExternalInput")
             for l in range(3)]
    gw_d = [nc.dram_tensor(f"gw{l}", [C, C], fp32, kind="ExternalInput")
            for l in range(2)]
    gb_d = [nc.dram_tensor(f"gb{l}", [C, 1], fp32, kind="ExternalInput")
            for l in range(2)]
    newT_d = nc.dram_tensor("newT", [64, 9 * NPC], fp32,
                            kind="ExternalOutput")
    if debug:
        dbg_xg = nc.dram_tensor("dbg_xg", [128, BP, GCOLP], fp32,
                                kind="ExternalOutput")
        dbg_w = nc.dram_tensor("dbg_w", [128, BP, W3COL], fp32,
                               kind="ExternalOutput")
        dbg_msgs = nc.dram_tensor("dbg_msgs", [128, BP, F], fp32,
                                  kind="ExternalOutput")
        dbg_agg = nc.dram_tensor("dbg_agg", [128, NW, 640], fp32,
                                 kind="ExternalOutput")

    with tile.TileContext(nc) as tc, ExitStack() as ctx:
        consts = ctx.enter_context(tc.tile_pool(name="consts", bufs=1))
        idx_sb = consts.tile([128, E_PAD // 16], mybir.dt.int16)
        nc.sync.dma_start(idx_sb[:], sidx[:])
        w1_sb = consts.tile([8, H], bf16)
        nc.sync.dma_start(w1_sb[:], w1_d[:])
        b1_sb = consts.tile([H, 1], fp32)
        nc.sync.dma_start(b1_sb[:], b1_d[:])
        w2_sb = consts.tile([H, H], bf16)
        nc.sync.dma_start(w2_sb[:], w2_d[:])
        b2_sb = consts.tile([H, 1], fp32)
        nc.sync.dma_start(b2_sb[:], b2_d[:])
        w3_sb = consts.tile([H, W3COL], bf16)
        nc.sync.dma_start(w3_sb[:], w3_d[:])
        lin_sb = [consts.tile([C, C], fp32, name=f"lin{l}", tag=f"lin{l}")
                  for l in range(3)]
        for l in range(3):
            nc.sync.dma_start(lin_sb[l][:], lin_d[l][:])
        gw_sb = [consts.tile([C, C], fp32, name=f"gw{l}", tag=f"gw{l}")
                 for l in range(2)]
        gb_sb = [consts.tile([C, 1], fp32, name=f"gb{l}", tag=f"gb{l}")
                 for l in range(2)]
        for l in range(2):
            nc.sync.dma_start(gw_sb[l][:], gw_d[l][:])
            nc.sync.dma_start(gb_sb[l][:], gb_d[l][:])
        agg_sb = consts.tile([64, NW, 2, 5, 128], fp32)

        with ExitStack() as psctx:
            iop = psctx.enter_context(tc.tile_pool(name="iop", bufs=2))
            aggt = psctx.enter_context(tc.tile_pool(name="aggt", bufs=1))
            wp = psctx.enter_context(tc.tile_pool(name="wp", bufs=1))
            msgp = psctx.enter_context(tc.tile_pool(name="msgp", bufs=2))
            shbp = psctx.enter_context(tc.tile_pool(name="shbp", bufs=1))
            pp = psctx.enter_context(tc.tile_pool(name="pp", bufs=1))
            zjp = psctx.enter_context(tc.tile_pool(name="zjp", bufs=1))
            zgp = psctx.enter_context(tc.tile_pool(name="zgp", bufs=1))
            h_ps = psctx.enter_context(
                tc.tile_pool(name="h_ps", bufs=1, space="PSUM"))
            w_ps_pool = psctx.enter_context(
                tc.tile_pool(name="w_ps", bufs=1, space="PSUM"))
            agg_pool = psctx.enter_context(
                tc.tile_pool(name="agg_ps", bufs=2, space="PSUM"))

            agg_open = {}

            for g in range(NG):
                t0 = g * BP
                xg = iop.tile([128, BP, GCOLP], bf16, tag="xg")
                nc.gpsimd.dma_gather(
                    out_ap=xg[:],
                    in_ap=ftab[:],
                    idxs_ap=idx_sb[:, g * (BP * 8):(g + 1) * (BP * 8)],
                    num_idxs=BP * 128,
                    num_idxs_reg=BP * 128,
                    elem_size=GCOLP,
                )
                shp_t = iop.tile([128, BP, 9], bf16, tag="shp")
                nc.sync.dma_start(shp_t[:], shp_d[:, t0:t0 + BP, :])
                if layer2:
                    shq_t = iop.tile([128, BP, NSHQ], bf16, tag="shq")
                    nc.sync.dma_start(shq_t[:], shq_d[:, t0:t0 + BP, :])
                smt = iop.tile([128, BP, 128], bf16, tag="smt")
                nc.sync.dma_start(smt[:], smat_d[:, t0:t0 + BP, :])
                bas = iop.tile([8, BP * 128], bf16, tag="bas")
                nc.sync.dma_start(bas[:], basT_d[g, :, :])

                if STAGE < 2:
                    continue
                if not layer2:
                    # sh broadcast table for L1 (ScalarE, stride-0 copy)
                    shB = shbp.tile([128, BP, 9, C], bf16, tag="shB")
                    nc.scalar.copy(
                        shB[:],
                        shp_t[:, :, :, None].broadcast_to((128, BP, 9, C)))

                # ---- radial MLP (transposed; fused Silu) ----
                h1s = iop.tile([H, BP * 128], bf16, tag="h1s")
                h2s = iop.tile([H, BP * 128], bf16, tag="h2s")
                sg = iop.tile([H, BP * 128], bf16, tag="sg")
                for c0 in range(0, BP * 128, 512):
                    h1p = h_ps.tile([H, 512], fp32, tag="h")
                    nc.tensor.matmul(h1p[:], w1_sb[:],
                                     bas[:, c0:c0 + 512], start=True, stop=True)
                    nc.scalar.activation(sg[:, c0:c0 + 512], h1p[:],
                                         AF.Sigmoid, bias=b1_sb[:, 0:1])
                    nc.vector.scalar_tensor_tensor(
                        out=h1s[:, c0:c0 + 512], in0=h1p[:],
                        scalar=b1_sb[:, 0:1], in1=sg[:, c0:c0 + 512],
                        op0=ALU.add, op1=ALU.mult)
                for c0 in range(0, BP * 128, 512):
                    h2p = h_ps.tile([H, 512], fp32, tag="h")
                    nc.tensor.matmul(h2p[:], w2_sb[:],
                                     h1s[:, c0:c0 + 512], start=True, stop=True)
                    nc.scalar.activation(sg[:, c0:c0 + 512], h2p[:],
                                         AF.Sigmoid, bias=b2_sb[:, 0:1])
                    nc.vector.scalar_tensor_tensor(
                        out=h2s[:, c0:c0 + 512], in0=h2p[:],
                        scalar=b2_sb[:, 0:1], in1=sg[:, c0:c0 + 512],
                        op0=ALU.add, op1=ALU.mult)

                # ---- per-tile edge weights w = h2s_t.T @ W3 (bf16 out) ----
                w_sb = wp.tile([128, BP, W3COL], bf16, tag="wsb")
                for bt in range(BP):
                    w_ps = w_ps_pool.tile([128, W3COL], fp32, tag="wps")
                    for c0 in range(0, W3COL, 512):
                        c1 = min(c0 + 512, W3COL)
                        nc.tensor.matmul(w_ps[:, c0:c1],
                                         h2s[:, bt * 128:(bt + 1) * 128],
                                         w3_sb[:, c0:c1], start=True, stop=True)
                    nc.scalar.copy(w_sb[:, bt, :], w_ps[:, 0:W3COL])

                if STAGE < 3:
                    continue
                # ---- tensor product ----
                msgs = msgp.tile([128, BP, F], bf16, tag="msgs")
                if layer2:
                    _emit_tp_l2(nc, ALU, xg, w_sb, shq_t, msgs, pp, zjp, zgp)
                else:
                    _emit_tp_l1(nc, ALU, xg, w_sb, shB, msgs, pp)

                if debug and g == 0:
                    dxg = pp.tile([128, BP, GCOLP], fp32, tag="dxg")
                    nc.vector.tensor_copy(out=dxg[:], in_=xg[:])
                    nc.sync.dma_start(dbg_xg[:], dxg[:])
                    dw = pp.tile([128, BP, W3COL], fp32, tag="dw")
                    nc.vector.tensor_copy(out=dw[:], in_=w_sb[:])
                    nc.sync.dma_start(dbg_w[:], dw[:])
                    dmg = pp.tile([128, BP, F], fp32, tag="dmg")
                    nc.vector.tensor_copy(out=dmg[:], in_=msgs[:])
                    nc.sync.dma_start(dbg_msgs[:], dmg[:])

                if STAGE < 4:
                    continue
                # ---- segment sum: paired-kg bf16 matmuls, PSUM-accumulated ----
                for bt in range(BP):
                    t = t0 + bt
                    w, t_in_w = t // tpw, t % tpw
                    if w not in agg_open:
                        agg_open[w] = agg_pool.tile([128, 640], fp32,
                                                    name="aggps", tag="aggps")
                    ps = agg_open[w]
                    first, last = t_in_w == 0, t_in_w == tpw - 1
                    for pair in range(5):
                        c0 = pair * 128
                        cw = 128 if pair < 4 else 64
                        nc.tensor.matmul(
                            ps[0:cw, c0:c0 + 128],
                            msgs[:, bt, c0:c0 + cw],
                            smt[:, bt, :],
                            start=first and pair in (0, 4),
                            stop=last, skip_group_check=True)
                    if last:
                        aps = agg_open.pop(w)
                        afp = aggt.tile([128, 640], fp32, tag="afp")
                        nc.scalar.copy(afp[:, 0:512], aps[:, 0:512])
                        nc.scalar.copy(afp[0:64, 512:640], aps[0:64, 512:640])
                        nc.sync.dma_start(
                            agg_sb[:, w, 0, :, :],
                            afp[0:64, :].rearrange("p (q n) -> p q n", q=5))
                        nc.sync.dma_start(
                            agg_sb[:, w, 1, 0:4, :],
                            afp[64:128, 0:512].rearrange("p (q n) -> p q n",
                                                         q=4))



        # ---------------- per-window node update ----------------
        if STAGE < 5:
            with ExitStack() as upctx:
                upt = upctx.enter_context(tc.tile_pool(name="upt", bufs=2))
                for w in range(NW):
                    tmp = upt.tile([64, 9, 128], fp32, tag="pass")
                    nc.sync.dma_start(
                        tmp[:], oldT_d[:, :].rearrange(
                            "p (q n) -> p q n",
                            q=9)[:, :, w * 128:(w + 1) * 128])
                    nc.sync.dma_start(
                        newT_d[:, :].rearrange(
                            "p (q n) -> p q n",
                            q=9)[:, :, w * 128:(w + 1) * 128],
                        tmp[:])
            nc.compile()
            return nc
        with ExitStack() as upctx:
            y_pool = upctx.enter_context(
                tc.tile_pool(name="y_ps", bufs=2, space="PSUM"))
            g_pool = upctx.enter_context(
                tc.tile_pool(name="g_ps", bufs=2, space="PSUM"))
            upt = upctx.enter_context(tc.tile_pool(name="upt", bufs=2))
            for w in range(NW):
                oldw = upt.tile([64, 9, 128], fp32, tag="oldw")
                nc.sync.dma_start(
                    oldw[:], oldT_d[:, :].rearrange(
                        "p (q n) -> p q n", q=9)[:, :, w * 128:(w + 1) * 128])
                neww = upt.tile([64, 9, 128], fp32, tag="neww")
                y_ps = y_pool.tile([64, 9 * 128], fp32, tag="yps")
                for kg in range(9):
                    l = 0 if kg == 0 else (1 if kg <= 3 else 2)
                    nc.tensor.matmul(
                        y_ps[:, kg * 128:(kg + 1) * 128],
                        lin_sb[l][:],
                        agg_sb[:, w, kg % 2, kg // 2, :],
                        start=kg in (0, 4, 8), stop=True,
                        skip_group_check=True)
                y0g = upt.tile([C, 128], fp32, tag="y0g")
                nc.scalar.activation(y0g[:], y_ps[:, 0:128], AF.Sigmoid)
                y0s = upt.tile([C, 128], fp32, tag="y0s")
                nc.vector.tensor_tensor(out=y0s[:], in0=y_ps[:, 0:128],
                                        in1=y0g[:], op=ALU.mult)
                nc.vector.tensor_tensor(out=neww[:, 0, :], in0=y0s[:],
                                        in1=oldw[:, 0, :], op=ALU.add)
                g_ps = g_pool.tile([C, 2, 128], fp32, tag="gps")
                for l in (1, 2):
                    nc.tensor.matmul(g_ps[:, l - 1, :], gw_sb[l - 1][:],
                                     neww[:, 0, :], start=(l == 1), stop=True,
                                     skip_group_check=True)
                gts = upt.tile([C, 2, 128], fp32, tag="gts")
                for l in (1, 2):
                    nc.scalar.activation(gts[:, l - 1, :], g_ps[:, l - 1, :],
                                         AF.Sigmoid, bias=gb_sb[l - 1][:, 0:1])
                gy = upt.tile([C, 8, 128], fp32, tag="gy")
                nc.vector.tensor_tensor(
                    out=gy[:, 0:3, :],
                    in0=y_ps[:].rearrange("p (q n) -> p q n", q=9)[:, 1:4, :],
                    in1=gts[:, 0:1, :].broadcast_to((C, 3, 128)),
                    op=ALU.mult)
                nc.vector.tensor_tensor(
                    out=gy[:, 3:8, :],
                    in0=y_ps[:].rearrange("p (q n) -> p q n", q=9)[:, 4:9, :],
                    in1=gts[:, 1:2, :].broadcast_to((C, 5, 128)),
                    op=ALU.mult)
                nc.vector.tensor_tensor(out=neww[:, 1:9, :], in0=gy[:],
                                        in1=oldw[:, 1:9, :], op=ALU.add)
                nc.sync.dma_start(
                    newT_d[:, :].rearrange("p (q n) -> p q n",
                                           q=9)[:, :, w * 128:(w + 1) * 128],
                    neww[:])

    nc.compile()
    return nc


def _emit_tp_l2(nc, ALU, xg, w_sb, shq_t, msgs, pp, zjp, zgp):
    """P products, cg-folded zjQ scalings, FMA as adds (+8 ratio stts).

    All on DVE: single in-order queue, no cross-engine ping-pong; GpSimd
    has ~9us fixed cost per elementwise op and ScalarE cannot multiply
    two tensors, so the TP bulk lives here.
    """
    import concourse.mybir as mybir
    bf16 = mybir.dt.bfloat16

    GP_JS = (8,)        # l2=2 zjQ op offloaded to GpSimd (big ops only)
    P0 = pp.tile([128, BP, SCHED2_L2[0]["nblk"] * C], bf16, name="P0", tag="P0")
    P12 = pp.tile([128, BP, SCHED2_L2[2]["nblk"] * C], bf16,
                  name="P12", tag="P12")
    zj12 = zjp.tile([128, BP, SCHED2_L2[2]["nblk"] * C], bf16,
                    name="zj12", tag="zj12")
    zjG = {j: zgp.tile([128, BP, SCHED2_L2[2]["nblk"] * C], bf16,
                       name=f"zjG{j}", tag=f"zjG{j}") for j in GP_JS}

    def emit_zjq(eng, dst, P, qoff, segs):
        for (z0, L) in segs:
            eng.tensor_tensor(
                out=dst[:].rearrange("p b (n c) -> p b n c",
                                     c=C)[:, :, z0:z0 + L, :],
                in0=P[:].rearrange("p b (n c) -> p b n c",
                                   c=C)[:, :, z0:z0 + L, :],
                in1=shq_t[:, :, qoff + z0:qoff + z0 + L, None].broadcast_to(
                    (128, BP, L, C)),
                op=ALU.mult)

    def emit_fma(sch, j, zj):
        for (jj, z0, k0, L) in sch["runs_add"]:
            if jj != j:
                continue
            nc.vector.tensor_tensor(
                out=msgs[:, :, k0 * C:(k0 + L) * C],
                in0=zj[:, :, z0 * C:(z0 + L) * C],
                in1=msgs[:, :, k0 * C:(k0 + L) * C],
                op=ALU.add)
        for (jj, z, k, ratio) in sch["runs_stt"]:
            if jj != j:
                continue
            nc.vector.scalar_tensor_tensor(
                out=msgs[:, :, k * C:(k + 1) * C],
                in0=zj[:, :, z * C:(z + 1) * C],
                scalar=float(ratio),
                in1=msgs[:, :, k * C:(k + 1) * C],
                op0=ALU.mult, op1=ALU.add)

    for gi, sch in enumerate(SCHED2_L2):
        nblk = sch["nblk"]
        P = P0 if gi == 0 else P12
        for (p, ig0, ni, z0) in sch["pops"]:
            nc.vector.tensor_tensor(
                out=P[:].rearrange("p b (n c) -> p b n c",
                                   c=C)[:, :, z0:z0 + ni, :],
                in0=xg[:].rearrange("p b (n c) -> p b n c",
                                    c=C)[:, :, ig0:ig0 + ni, :],
                in1=w_sb[:, :, None, p * C:(p + 1) * C].broadcast_to(
                    (128, BP, ni, C)),
                op=ALU.mult)
        if gi > 0:
            # kick GpSimd's share first so it overlaps DVE's other j's
            for (j, qoff, segs) in sch["zjq"]:
                if j in GP_JS:
                    emit_zjq(nc.gpsimd, zjG[j], P, qoff, segs)
        for (j, qoff, segs) in sch["zjq"]:
            if gi > 0 and j in GP_JS:
                continue
            dst = msgs if gi == 0 else zj12
            emit_zjq(nc.vector, dst, P, qoff, segs)
            if gi == 0:
                continue
            emit_fma(sch, j, zj12)
        if gi > 0:
            for j in GP_JS:
                if any(jj == j for (jj, _, _) in sch["zjq"]):
                    emit_fma(sch, j, zjG[j])


def _emit_tp_l1(nc, ALU, xg, w_sb, shB, msgs, pp):
    """msgs[k] = x * w'_k * sh_k ; w' host-expanded+cg-folded to 9 kg blocks."""
    import concourse.mybir as mybir
    bf16 = mybir.dt.bfloat16
    tmp = pp.tile([128, BP, 9, C], bf16, name="l1tmp", tag="l1tmp")
    nc.vector.tensor_tensor(
        out=tmp[:],
        in0=w_sb[:].rearrange("p b (n c) -> p b n c", c=C),
        in1=shB[:],
        op=ALU.mult)
    nc.vector.tensor_tensor(
        out=msgs[:].rearrange("p b (n c) -> p b n c", c=C),
        in0=tmp[:],
        in1=xg[:, :, None, 0:C].broadcast_to((128, BP, 9, C)),
        op=ALU.mult)


# ---------------- host orchestration ----------------
def _chunked_T(feats_own):
    """[NPC, 576] -> kg-blocked transposed [64, 9*NPC]."""
    out = np.empty((64, 9 * NPC), np.float32)
    for kg in range(9):
        out[:, kg * NPC:(kg + 1) * NPC] = feats_own[:, kg * 64:(kg + 1) * 64].T
    return out


def _unchunk_T(newT):
    """[64, 9*NPC] -> [NPC, 576]."""
    out = np.empty((NPC, 576), np.float32)
    for kg in range(9):
        out[:, kg * 64:(kg + 1) * 64] = newT[:, kg * NPC:(kg + 1) * NPC].T
    return out


_CACHE = {}


def _prep(positions, senders, receivers):
    key = (senders.tobytes(), receivers.tobytes(), positions.tobytes())
    if _CACHE.get("key") == key:
        return _CACHE["val"]
    sh_eff, basis = edge_geometry(positions, senders, receivers)
    owner, local, nodes_of, _ = partition_graph(receivers)
    deg_bin = np.zeros(NCORES * NW, np.int64)
    np.add.at(deg_bin, owner[receivers] * NW + local[receivers] // 128, 1)
    tpw = (int(deg_bin.max()) + 127) // 128
    T = NW * tpw
    assert T % BP == 0
    perm = build_core_edges(receivers, owner, local, tpw)

    valid = perm >= 0
    eg = np.where(valid, perm, 0)
    snd = np.where(valid, senders[eg], 0).astype(np.int16)      # [NC, T*128]
    shp_e = sh_eff[eg] * valid[..., None]                        # [NC, T*128, 9]
    bas_e = basis[eg] * valid[..., None]                         # [NC, T*128, 8]
    lr = np.where(valid, local[receivers[eg]], 0)

    NG = T // BP
    inv = np.float32(1.0 / np.sqrt(AVG_NN))
    sidx = np.empty((NCORES, 128, T * 128 // 16), np.int16)
    shp_h = np.empty((NCORES, 128, T, 9), BF16)
    shq_h = np.empty((NCORES, 128, T, NSHQ), BF16)
    jcols = np.array([j for (j, cg) in SHQ_COLS])
    cgv = np.array([cg for (j, cg) in SHQ_COLS], np.float32)
    bas_h = np.empty((NCORES, NG, 8, BP * 128), BF16)
    smat_h = np.zeros((NCORES, 128, T, 128), BF16)
    for k in range(NCORES):
        s = snd[k].reshape(T * 8, 16)
        sidx[k] = np.tile(s.T, (8, 1))
        shp_f = shp_e[k].reshape(T, 128, 9).transpose(1, 0, 2)
        shp_h[k] = shp_f.astype(BF16)
        shq_h[k] = (shp_f[:, :, jcols] * cgv[None, None, :]).astype(BF16)
        bas_h[k] = bas_e[k].reshape(NG, BP * 128, 8).transpose(0, 2, 1).astype(BF16)
        v = valid[k]
        e_slots = np.arange(T * 128)
        p_, t_ = e_slots % 128, e_slots // 128
        cols = lr[k] - (t_ // tpw) * 128
        ok = v & (cols >= 0) & (cols < 128)
        sm = np.zeros((128, T, 128), np.float32)
        sm[p_[ok], t_[ok], cols[ok]] = inv
        smat_h[k] = sm.astype(BF16)
    val = dict(T=T, NG=NG, tpw=tpw, nodes_of=nodes_of, sidx=sidx,
               shp_h=shp_h, bas_h=bas_h, smat_h=smat_h, shq_h=shq_h)
    _CACHE["key"], _CACHE["val"] = key, val
    return val


EXEC_NS = []


def _run_layer(nc, pre, ftab, oldT_by_core, lw):
    import os
    from concourse.bass_utils import run_bass_kernel_spmd
    in_maps = []
    for k in range(NCORES):
        m = dict(ftab=ftab,
                 sidx=pre["sidx"][k],
                 shp=pre["shp_h"][k],
                 shq=pre["shq_h"][k],
                 basisT=pre["bas_h"][k],
                 smat=pre["smat_h"][k],
                 oldT=oldT_by_core[k],
                 w1=lw["w1"], b1=lw["b1"], w2=lw["w2"], b2=lw["b2"],
                 w3=lw["w3"], lin0=lw["lin"][0], lin1=lw["lin"][1],
                 lin2=lw["lin"][2], gw0=lw["gw"][0], gw1=lw["gw"][1],
                 gb0=lw["gb"][0], gb1=lw["gb"][1])
        in_maps.append(m)
    trace = bool(os.environ.get("KERNEL_TRACE"))
    res = run_bass_kernel_spmd(nc, in_maps, list(range(NCORES)), trace=trace,
                               trace_cores=list(range(NCORES)) if trace else None)
    if trace and res.exec_time_ns is not None:
        EXEC_NS.append(res.exec_time_ns)
    return [res.results[k]["newT"] for k in range(NCORES)]


def _layer_weights(inputs, i, layer2):
    f32 = np.float32
    w3 = np.ascontiguousarray(inputs["mlp_w3"][i], f32)
    if layer2:
        w3p = w3.astype(BF16)
    else:
        w3p = np.empty((H, 576), BF16)
        for kg in range(9):
            p = L1_PATH_OF_K[kg]
            w3p[:, kg * C:(kg + 1) * C] = (
                w3[:, p * C:(p + 1) * C] * L1_CG_OF_K[kg]).astype(BF16)
    return dict(
        w1=np.ascontiguousarray(inputs["mlp_w1"][i], f32).astype(BF16),
        b1=np.ascontiguousarray(inputs["mlp_b1"][i], f32).reshape(H, 1),
        w2=np.ascontiguousarray(inputs["mlp_w2"][i], f32).astype(BF16),
        b2=np.ascontiguousarray(inputs["mlp_b2"][i], f32).reshape(H, 1),
        w3=w3p,
        lin=[np.ascontiguousarray(inputs["lin_self"][i, l], f32)
             for l in range(3)],
        gw=[np.ascontiguousarray(inputs["gate_w"][i, l], f32)
            for l in range(2)],
        gb=[np.ascontiguousarray(inputs["gate_b"][i, l], f32).reshape(C, 1)
            for l in range(2)],
    )


_KERNEL_CACHE = {}


def _get_kernels(T):
    if T not in _KERNEL_CACHE:
        _KERNEL_CACHE[T] = (build_layer_kernel(False, T),
                            build_layer_kernel(True, T))
    return _KERNEL_CACHE[T]


def _pack_ftab(table, ncols):
    out = np.zeros((N_NODES, ncols), BF16)
    used = min(ncols, table.shape[1])
    out[:, :used] = table[:, :used].astype(BF16)
    return out


def kernel(**inputs):
    positions = np.asarray(inputs["positions"], np.float32)
    species = np.asarray(inputs["species"]).astype(np.int64)
    senders = np.asarray(inputs["senders"]).astype(np.int64)
    receivers = np.asarray(inputs["receivers"]).astype(np.int64)

    pre = _prep(positions, senders, receivers)
    T = pre["T"]
    nc1, nc2 = _get_kernels(T)
    nodes_of = pre["nodes_of"]

    # initial features: x0 from species embedding (host; tiny)
    x0 = (np.asarray(inputs["embed"], np.float32)[species]
          @ np.asarray(inputs["w_proj"], np.float32))          # [N, 64]
    table = np.zeros((N_NODES, F), np.float32)
    table[:, 0:C] = x0

    # ---- layer 1 ----
    oldT = [_chunked_T(table[nodes_of[k]]) for k in range(NCORES)]
    lw = _layer_weights(inputs, 0, False)
    newT = _run_layer(nc1, pre, _pack_ftab(table, GCOLP_L1), oldT, lw)

    table2 = np.empty((N_NODES, F), np.float32)
    for k in range(NCORES):
        table2[nodes_of[k]] = _unchunk_T(newT[k])

    # ---- layer 2 ----
    lw = _layer_weights(inputs, 1, True)
    newT2 = _run_layer(nc2, pre, _pack_ftab(table2, GCOLP_L2), newT, lw)

    table3 = np.empty((N_NODES, F), np.float32)
    for k in range(NCORES):
        table3[nodes_of[k]] = _unchunk_T(newT2[k])

    # ---- output: reorder component-major -> reference layout + alpha ----
    t3 = table3.reshape(N_NODES, 9, C)
    out = np.empty((N_NODES, F), np.float32)
    out[:, 0:64] = t3[:, 0]
    out[:, 64:256] = (0.5 * t3[:, 1:4]).transpose(0, 2, 1).reshape(N_NODES, 192)
    out[:, 256:576] = (0.25 * t3[:, 4:9]).transpose(0, 2, 1).reshape(N_NODES, 320)
    return out



# revision 20
# speedup vs baseline: 1.1720x; 1.0445x over previous
"""NequIP GNN message-passing kernel for 8 Trainium2 NeuronCores — v2.

Receiver-sharded graph parallelism (per sharding hint): host LPT-assigns the
8192 nodes to 64 (core, window) bins of 128, each core owns 8 windows = 1024
nodes + their in-edges, sorted by window, padded to 128-edge tiles.

v2 device pipeline (vs v1): bf16 edge pipeline end-to-end with fp32 PSUM
accumulation; per-edge-scalar stages batched across 8-tile groups via
stride-0 broadcast access patterns; P-products collapsed per-path (w operand
broadcast over l1 components); CG-FMA stage merged into diagonal runs with
immediate scalars; segment-sum via paired-kg bf16 matmuls; radial MLP with
fused Silu activations; elementwise work split across Vector/GpSimd/Scalar.
"""
import math
import numpy as np
import ml_dtypes

BF16 = ml_dtypes.bfloat16

# ---------------- model constants ----------------
N_NODES, N_EDGES = 8192, 131072
C, H, NRAD = 64, 64, 8
R_MAX, AVG_NN = 5.0, 16.0
NCORES, NPC = 8, 1024
NW = NPC // 128
F = 9 * C
LS = (0, 1, 2)
PATHS = [(l1, l2, l3) for l1 in LS for l2 in LS for l3 in LS
         if abs(l1 - l2) <= l3 <= l1 + l2]
LOFF = {0: 0, 1: 1, 2: 4}
J_OF_L2 = {0: [0], 1: [1, 2, 3], 2: [4, 5, 6, 7, 8]}
BP = 8                      # tiles per group (batched in op free dims)
GCOLP_L2 = 640              # padded gather row (bf16): 1280B % 256 == 0
GCOLP_L1 = 128              # 256B % 256 == 0


# ---------------- real Clebsch-Gordan coefficients ----------------
def _cg_scalar(j1, m1, j2, m2, j3, m3):
    f = math.factorial
    if m1 + m2 != m3:
        return 0.0
    pre = ((2*j3+1) * f(j1+j2-j3) * f(j1-j2+j3) * f(-j1+j2+j3)
           / f(j1+j2+j3+1)) ** 0.5
    pre *= (f(j1+m1)*f(j1-m1)*f(j2+m2)*f(j2-m2)*f(j3+m3)*f(j3-m3)) ** 0.5
    s = 0.0
    for k in range(max(0, j2-j3-m1, j1+m2-j3), min(j1+j2-j3, j1-m1, j2+m2)+1):
        s += (-1)**k / (f(k)*f(j1+j2-j3-k)*f(j1-m1-k)
                        * f(j2+m2-k)*f(j3-j2+m1+k)*f(j3-j1-m2+k))
    return pre * s


def _U_real(l):
    U = np.zeros((2*l+1, 2*l+1), dtype=complex)
    s2 = 2 ** -0.5
    for m in range(-l, l+1):
        if m > 0:
            U[m+l, m+l] = (-1)**m * s2
            U[m+l, -m+l] = s2
        elif m == 0:
            U[l, l] = 1.0
        else:
            U[m+l, m+l] = 1j*s2
            U[m+l, -m+l] = -1j*(-1)**(-m)*s2
    return U


def _real_cg(l1, l2, l3):
    Cc = np.zeros((2*l1+1, 2*l2+1, 2*l3+1))
    for i1, m1 in enumerate(range(-l1, l1+1)):
        for i2, m2 in enumerate(range(-l2, l2+1)):
            m3 = m1 + m2
            if abs(m3) <= l3:
                Cc[i1, i2, m3+l3] = _cg_scalar(l1, m1, l2, m2, l3, m3)
    U1, U2, U3 = _U_real(l1), _U_real(l2), _U_real(l3)
    W = np.einsum('ia,jb,kc,abc->ijk', U1.conj(), U2.conj(), U3,
                  Cc.astype(complex))
    W = W.real if np.linalg.norm(W.real) >= np.linalg.norm(W.imag) else W.imag
    W = W / np.linalg.norm(W) * (2*l3+1) ** 0.5
    return np.asarray(W, dtype=np.float64)


CGS = [_real_cg(*p) for p in PATHS]


def build_schedule_l2():
    """Static TP structure for layer 2, grouped by l2.

    Per group: nblk, pops=[(path, ig0, ni, z0)], jlist, runs=[(j,z0,k0,L,cg)].
    """
    scheds = []
    for l2 in (0, 1, 2):
        ps = [p for p in range(len(PATHS)) if PATHS[p][1] == l2]
        blocks, block_of, pops = [], {}, []
        for p in ps:
            l1 = PATHS[p][0]
            ni = 2 * l1 + 1
            pops.append((p, LOFF[l1], ni, len(blocks)))
            for i in range(ni):
                block_of[(p, i)] = len(blocks)
                blocks.append((p, i))
        tset = set()
        for p in ps:
            l1, _, l3 = PATHS[p]
            cg = CGS[p]
            for i in range(2*l1+1):
                for j in range(2*l2+1):
                    for k in range(2*l3+1):
                        v = cg[i, j, k]
                        if abs(v) > 1e-12:
                            tset.add((LOFF[l2]+j, block_of[(p, i)],
                                      LOFF[l3]+k, round(float(v), 9)))
        runs, consumed = [], set()
        for t in sorted(tset):
            if t in consumed:
                continue
            j, z, k, cgv = t
            L = 0
            while (j, z+L, k+L, cgv) in tset and (j, z+L, k+L, cgv) not in consumed:
                consumed.add((j, z+L, k+L, cgv))
                L += 1
            runs.append((j, z, k, L, cgv))
        scheds.append(dict(l2=l2, nblk=len(blocks), pops=pops,
                           jlist=J_OF_L2[l2], runs=runs))
    return scheds


SCHED_L2 = build_schedule_l2()


def build_schedule2():
    """cg folded into per-(j,z) host scalars; FMA mostly tensor adds.

    Per group: nblk, pops, zjq=[(j, qoff)], runs_add=[(j,z0,k0,L)],
    runs_stt=[(j,z,k,ratio)]. qoff = column offset of (j,*) block in the
    concatenated shq table [128, T, 179]. cgfold[qoff+z] = cg of the primary
    (smallest-k) triple of (j,z); 0 for unused (z,j).
    """
    scheds, qoff, cgcols = [], 0, []
    for gi, sch in enumerate(SCHED_L2):
        nblk = sch["nblk"]
        tset = set()
        for (j, z, k, L, cg) in sch["runs"]:
            for i in range(L):
                tset.add((j, z + i, k + i, cg))
        per_jz = {}
        for (j, z, k, cg) in tset:
            per_jz.setdefault((j, z), []).append((k, cg))
        primary, runs_stt = set(), []
        cgf = {}
        for (j, z), ks in per_jz.items():
            ks.sort()
            k0, cg0 = ks[0]
            primary.add((j, z, k0))
            cgf[(j, z)] = cg0
            for (k1, cg1) in ks[1:]:
                runs_stt.append((j, z, k1, cg1 / cg0))
        runs_add, consumed = [], set()
        for t in sorted(primary):
            if t in consumed:
                continue
            j, z, k = t
            L = 0
            while (j, z + L, k + L) in primary and (j, z + L, k + L) not in consumed:
                consumed.add((j, z + L, k + L))
                L += 1
            runs_add.append((j, z, k, L))
        zjq = []
        for j in sch["jlist"]:
            # contiguous segments of z columns actually used by (j, z):
            # skips the ~28% of dense columns with cg == 0
            used = sorted(z for (jj, z) in cgf if jj == j)
            segs, s0, prev = [], None, None
            for z in used:
                if s0 is None:
                    s0 = prev = z
                elif z == prev + 1:
                    prev = z
                else:
                    segs.append((s0, prev - s0 + 1))
                    s0 = prev = z
            if s0 is not None:
                segs.append((s0, prev - s0 + 1))
            zjq.append((j, qoff, segs))
            for z in range(nblk):
                cgcols.append((j, cgf.get((j, z), 0.0)))
            qoff += nblk
        scheds.append(dict(nblk=nblk, pops=sch["pops"], zjq=zjq,
                           runs_add=runs_add, runs_stt=runs_stt))
    return scheds, cgcols


SCHED2_L2, SHQ_COLS = build_schedule2()
NSHQ = len(SHQ_COLS)

# layer-1 per-k path and cg (x is scalar-only: paths (0,l,l), j == k)
L1_PATH_OF_K = [0] + [1]*3 + [2]*5
L1_CG_OF_K = []
for _k in range(9):
    _p = L1_PATH_OF_K[_k]
    _l = PATHS[_p][2]
    _m = _k - LOFF[_l]
    L1_CG_OF_K.append(float(CGS[_p][0, _m, _m]))


# ---------------- host-side graph preprocessing ----------------
def edge_geometry(positions, senders, receivers):
    rel = (positions[receivers] - positions[senders]) / R_MAX
    d = np.linalg.norm(rel, axis=-1)
    u = rel / np.maximum(d, 1e-6)[:, None]
    x, y, z = u[:, 0], u[:, 1], u[:, 2]
    sh = np.empty((len(d), 9), np.float32)
    sh[:, 0] = 1.0
    sh[:, 1] = np.sqrt(3.0) * y
    sh[:, 2] = np.sqrt(3.0) * z
    sh[:, 3] = np.sqrt(3.0) * x
    sh[:, 4] = np.sqrt(15.0) * x * y
    sh[:, 5] = np.sqrt(15.0) * y * z
    sh[:, 6] = np.sqrt(5.0) / 2 * (3 * z * z - 1.0)
    sh[:, 7] = np.sqrt(15.0) * x * z
    sh[:, 8] = np.sqrt(15.0) / 2 * (x * x - y * y)
    freqs = np.arange(1, NRAD + 1, dtype=np.float64)
    xr = np.clip(d, 1e-4, 1.0)[:, None].astype(np.float64)
    basis = (np.sqrt(2.0) * np.sin(freqs * np.pi * xr) / xr).astype(np.float32)
    cut = (0.5 * (np.cos(np.pi * np.clip(d, 0.0, 1.0)) + 1.0)).astype(np.float32)
    return (sh * cut[:, None]).astype(np.float32), basis


def partition_graph(receivers):
    import heapq
    deg = np.bincount(receivers, minlength=N_NODES)
    order = np.argsort(-deg, kind="stable")
    nbins = NCORES * NW
    load = np.zeros(nbins, np.int64)
    cnt = np.zeros(nbins, np.int64)
    owner = np.empty(N_NODES, np.int32)
    local = np.empty(N_NODES, np.int32)
    heap = [(0, b) for b in range(nbins)]
    heapq.heapify(heap)
    for n in order:
        while True:
            l, b = heapq.heappop(heap)
            if cnt[b] < 128:
                break
        owner[n] = b // NW
        local[n] = (b % NW) * 128 + cnt[b]
        cnt[b] += 1
        load[b] += deg[n]
        if cnt[b] < 128:
            heapq.heappush(heap, (int(load[b]), b))
    nodes_of = np.empty((NCORES, NPC), np.int64)
    for n in range(N_NODES):
        nodes_of[owner[n], local[n]] = n
    return owner, local, nodes_of, int(load.max())


def build_core_edges(receivers, owner, local, tpw):
    T = NW * tpw
    perm = np.full((NCORES, T * 128), -1, np.int64)
    for k in range(NCORES):
        eids = np.where(owner[receivers] == k)[0]
        lr = local[receivers[eids]]
        o = np.argsort(lr, kind="stable")
        eids, lr = eids[o], lr[o]
        w_of = lr // 128
        for w in range(NW):
            sel = eids[w_of == w]
            assert len(sel) <= tpw * 128, "tiles-per-window overflow"
            base = w * tpw * 128
            perm[k, base:base + len(sel)] = sel
    return perm


# ---------------- bass kernel builder ----------------
def build_layer_kernel(layer2, T, debug=False):
    import concourse.bass as bass
    import concourse.bacc as bacc
    import concourse.tile as tile
    import concourse.mybir as mybir
    from contextlib import ExitStack

    fp32 = mybir.dt.float32
    bf16 = mybir.dt.bfloat16
    AF = mybir.ActivationFunctionType
    ALU = mybir.AluOpType

    NPATH = 15 if layer2 else 3
    GCOLP = GCOLP_L2 if layer2 else GCOLP_L1
    W3COL = NPATH * C if layer2 else 576   # L1 w3 host-expanded to 9 kg blocks
    E_PAD = T * 128
    NG = T // BP
    assert T % BP == 0 and T % NW == 0
    tpw = T // NW

    import os as _os
    STAGE = int(_os.environ.get("KV2_STAGE", "5"))
    nc = bacc.Bacc("TRN2", target_bir_lowering=False)

    ftab = nc.dram_tensor("ftab", [N_NODES, GCOLP], bf16, kind="ExternalInput")
    sidx = nc.dram_tensor("sidx", [128, E_PAD // 16], mybir.dt.int16,
                          kind="ExternalInput")
    shp_d = nc.dram_tensor("shp", [128, T, 9], bf16, kind="ExternalInput")
    shq_d = nc.dram_tensor("shq", [128, T, NSHQ], bf16, kind="ExternalInput")
    basT_d = nc.dram_tensor("basisT", [NG, 8, BP * 128], bf16,
                            kind="ExternalInput")
    smat_d = nc.dram_tensor("smat", [128, T, 128], bf16, kind="ExternalInput")
    oldT_d = nc.dram_tensor("oldT", [64, 9 * NPC], fp32, kind="ExternalInput")
    w1_d = nc.dram_tensor("w1", [8, H], bf16, kind="ExternalInput")
    b1_d = nc.dram_tensor("b1", [H, 1], fp32, kind="ExternalInput")
    w2_d = nc.dram_tensor("w2", [H, H], bf16, kind="ExternalInput")
    b2_d = nc.dram_tensor("b2", [H, 1], fp32, kind="ExternalInput")
    w3_d = nc.dram_tensor("w3", [H, W3COL], bf16, kind="ExternalInput")
    lin_d = [nc.dram_tensor(f"lin{l}", [C, C], fp32, kind="ExternalInput")
             for l in range(3)]
    gw_d = [nc.dram_tensor(f"gw{l}", [C, C], fp32, kind="ExternalInput")
            for l in range(2)]
    gb_d = [nc.dram_tensor(f"gb{l}", [C, 1], fp32, kind="ExternalInput")
            for l in range(2)]
    newT_d = nc.dram_tensor("newT", [64, 9 * NPC], fp32,
                            kind="ExternalOutput")
    if debug:
        dbg_xg = nc.dram_tensor("dbg_xg", [128, BP, GCOLP], fp32,
                                kind="ExternalOutput")
        dbg_w = nc.dram_tensor("dbg_w", [128, BP, W3COL], fp32,
                               kind="ExternalOutput")
        dbg_msgs = nc.dram_tensor("dbg_msgs", [128, BP, F], fp32,
                                  kind="ExternalOutput")
        dbg_agg = nc.dram_tensor("dbg_agg", [128, NW, 640], fp32,
                                 kind="ExternalOutput")

    with tile.TileContext(nc) as tc, ExitStack() as ctx:
        consts = ctx.enter_context(tc.tile_pool(name="consts", bufs=1))
        idx_sb = consts.tile([128, E_PAD // 16], mybir.dt.int16)
        nc.sync.dma_start(idx_sb[:], sidx[:])
        w1_sb = consts.tile([8, H], bf16)
        nc.sync.dma_start(w1_sb[:], w1_d[:])
        b1_sb = consts.tile([H, 1], fp32)
        nc.sync.dma_start(b1_sb[:], b1_d[:])
        w2_sb = consts.tile([H, H], bf16)
        nc.sync.dma_start(w2_sb[:], w2_d[:])
        b2_sb = consts.tile([H, 1], fp32)
        nc.sync.dma_start(b2_sb[:], b2_d[:])
        w3_sb = consts.tile([H, W3COL], bf16)
        nc.sync.dma_start(w3_sb[:], w3_d[:])
        lin_sb = [consts.tile([C, C], fp32, name=f"lin{l}", tag=f"lin{l}")
                  for l in range(3)]
        for l in range(3):
            nc.sync.dma_start(lin_sb[l][:], lin_d[l][:])
        gw_sb = [consts.tile([C, C], fp32, name=f"gw{l}", tag=f"gw{l}")
                 for l in range(2)]
        gb_sb = [consts.tile([C, 1], fp32, name=f"gb{l}", tag=f"gb{l}")
                 for l in range(2)]
        for l in range(2):
            nc.sync.dma_start(gw_sb[l][:], gw_d[l][:])
            nc.sync.dma_start(gb_sb[l][:], gb_d[l][:])
        agg_sb = consts.tile([64, NW, 2, 5, 128], fp32)

        with ExitStack() as psctx:
            iop = psctx.enter_context(tc.tile_pool(name="iop", bufs=2))
            aggt = psctx.enter_context(tc.tile_pool(name="aggt", bufs=1))
            wp = psctx.enter_context(tc.tile_pool(name="wp", bufs=1))
            msgp = psctx.enter_context(tc.tile_pool(name="msgp", bufs=2))
            shbp = psctx.enter_context(tc.tile_pool(name="shbp", bufs=1))
            pp = psctx.enter_context(tc.tile_pool(name="pp", bufs=1))
            zjp = psctx.enter_context(tc.tile_pool(name="zjp", bufs=1))
            zgp = psctx.enter_context(tc.tile_pool(name="zgp", bufs=1))
            h_ps = psctx.enter_context(
                tc.tile_pool(name="h_ps", bufs=1, space="PSUM"))
            w_ps_pool = psctx.enter_context(
                tc.tile_pool(name="w_ps", bufs=1, space="PSUM"))
            agg_pool = psctx.enter_context(
                tc.tile_pool(name="agg_ps", bufs=2, space="PSUM"))

            agg_open = {}

            for g in range(NG):
                t0 = g * BP
                xg = iop.tile([128, BP, GCOLP], bf16, tag="xg")
                nc.gpsimd.dma_gather(
                    out_ap=xg[:],
                    in_ap=ftab[:],
                    idxs_ap=idx_sb[:, g * (BP * 8):(g + 1) * (BP * 8)],
                    num_idxs=BP * 128,
                    num_idxs_reg=BP * 128,
                    elem_size=GCOLP,
                )
                shp_t = iop.tile([128, BP, 9], bf16, tag="shp")
                nc.sync.dma_start(shp_t[:], shp_d[:, t0:t0 + BP, :])
                if layer2:
                    shq_t = iop.tile([128, BP, NSHQ], bf16, tag="shq")
                    nc.sync.dma_start(shq_t[:], shq_d[:, t0:t0 + BP, :])
                smt = iop.tile([128, BP, 128], bf16, tag="smt")
                nc.sync.dma_start(smt[:], smat_d[:, t0:t0 + BP, :])
                bas = iop.tile([8, BP * 128], bf16, tag="bas")
                nc.sync.dma_start(bas[:], basT_d[g, :, :])

                if STAGE < 2:
                    continue
                if not layer2:
                    # sh broadcast table for L1 (ScalarE, stride-0 copy)
                    shB = shbp.tile([128, BP, 9, C], bf16, tag="shB")
                    nc.scalar.copy(
                        shB[:],
                        shp_t[:, :, :, None].broadcast_to((128, BP, 9, C)))

                # ---- radial MLP (transposed; fused Silu) ----
                h1s = iop.tile([H, BP * 128], bf16, tag="h1s")
                h2s = iop.tile([H, BP * 128], bf16, tag="h2s")
                sg = iop.tile([H, BP * 128], bf16, tag="sg")
                for c0 in range(0, BP * 128, 512):
                    h1p = h_ps.tile([H, 512], fp32, tag="h")
                    nc.tensor.matmul(h1p[:], w1_sb[:],
                                     bas[:, c0:c0 + 512], start=True, stop=True)
                    nc.scalar.activation(sg[:, c0:c0 + 512], h1p[:],
                                         AF.Sigmoid, bias=b1_sb[:, 0:1])
                    nc.vector.scalar_tensor_tensor(
                        out=h1s[:, c0:c0 + 512], in0=h1p[:],
                        scalar=b1_sb[:, 0:1], in1=sg[:, c0:c0 + 512],
                        op0=ALU.add, op1=ALU.mult)
                for c0 in range(0, BP * 128, 512):
                    h2p = h_ps.tile([H, 512], fp32, tag="h")
                    nc.tensor.matmul(h2p[:], w2_sb[:],
                                     h1s[:, c0:c0 + 512], start=True, stop=True)
                    nc.scalar.activation(sg[:, c0:c0 + 512], h2p[:],
                                         AF.Sigmoid, bias=b2_sb[:, 0:1])
                    nc.vector.scalar_tensor_tensor(
                        out=h2s[:, c0:c0 + 512], in0=h2p[:],
                        scalar=b2_sb[:, 0:1], in1=sg[:, c0:c0 + 512],
                        op0=ALU.add, op1=ALU.mult)

                # ---- per-tile edge weights w = h2s_t.T @ W3 (bf16 out) ----
                w_sb = wp.tile([128, BP, W3COL], bf16, tag="wsb")
                for bt in range(BP):
                    w_ps = w_ps_pool.tile([128, W3COL], fp32, tag="wps")
                    for c0 in range(0, W3COL, 512):
                        c1 = min(c0 + 512, W3COL)
                        nc.tensor.matmul(w_ps[:, c0:c1],
                                         h2s[:, bt * 128:(bt + 1) * 128],
                                         w3_sb[:, c0:c1], start=True, stop=True)
                    nc.scalar.copy(w_sb[:, bt, :], w_ps[:, 0:W3COL])

                if STAGE < 3:
                    continue
                # ---- tensor product ----
                msgs = msgp.tile([128, BP, F], bf16, tag="msgs")
                if layer2:
                    _emit_tp_l2(nc, ALU, xg, w_sb, shq_t, msgs, pp, zjp, zgp)
                else:
                    _emit_tp_l1(nc, ALU, xg, w_sb, shB, msgs, pp)

                if debug and g == 0:
                    dxg = pp.tile([128, BP, GCOLP], fp32, tag="dxg")
                    nc.vector.tensor_copy(out=dxg[:], in_=xg[:])
                    nc.sync.dma_start(dbg_xg[:], dxg[:])
                    dw = pp.tile([128, BP, W3COL], fp32, tag="dw")
                    nc.vector.tensor_copy(out=dw[:], in_=w_sb[:])
                    nc.sync.dma_start(dbg_w[:], dw[:])
                    dmg = pp.tile([128, BP, F], fp32, tag="dmg")
                    nc.vector.tensor_copy(out=dmg[:], in_=msgs[:])
                    nc.sync.dma_start(dbg_msgs[:], dmg[:])

                if STAGE < 4:
                    continue
                # ---- segment sum: paired-kg bf16 matmuls, PSUM-accumulated ----
                for bt in range(BP):
                    t = t0 + bt
                    w, t_in_w = t // tpw, t % tpw
                    if w not in agg_open:
                        agg_open[w] = agg_pool.tile([128, 640], fp32,
                                                    name="aggps", tag="aggps")
                    ps = agg_open[w]
                    first, last = t_in_w == 0, t_in_w == tpw - 1
                    for pair in range(5):
                        c0 = pair * 128
                        cw = 128 if pair < 4 else 64
                        nc.tensor.matmul(
                            ps[0:cw, c0:c0 + 128],
                            msgs[:, bt, c0:c0 + cw],
                            smt[:, bt, :],
                            start=first and pair in (0, 4),
                            stop=last, skip_group_check=True)
                    if last:
                        aps = agg_open.pop(w)
                        afp = aggt.tile([128, 640], fp32, tag="afp")
                        nc.scalar.copy(afp[:, 0:512], aps[:, 0:512])
                        nc.scalar.copy(afp[0:64, 512:640], aps[0:64, 512:640])
                        nc.sync.dma_start(
                            agg_sb[:, w, 0, :, :],
                            afp[0:64, :].rearrange("p (q n) -> p q n", q=5))
                        nc.sync.dma_start(
                            agg_sb[:, w, 1, 0:4, :],
                            afp[64:128, 0:512].rearrange("p (q n) -> p q n",
                                                         q=4))



        # ---------------- per-window node update ----------------
        if STAGE < 5:
            with ExitStack() as upctx:
                upt = upctx.enter_context(tc.tile_pool(name="upt", bufs=2))
                for w in range(NW):
                    tmp = upt.tile([64, 9, 128], fp32, tag="pass")
                    nc.sync.dma_start(
                        tmp[:], oldT_d[:, :].rearrange(
                            "p (q n) -> p q n",
                            q=9)[:, :, w * 128:(w + 1) * 128])
                    nc.sync.dma_start(
                        newT_d[:, :].rearrange(
                            "p (q n) -> p q n",
                            q=9)[:, :, w * 128:(w + 1) * 128],
                        tmp[:])
            nc.compile()
            return nc
        with ExitStack() as upctx:
            y_pool = upctx.enter_context(
                tc.tile_pool(name="y_ps", bufs=2, space="PSUM"))
            g_pool = upctx.enter_context(
                tc.tile_pool(name="g_ps", bufs=2, space="PSUM"))
            upt = upctx.enter_context(tc.tile_pool(name="upt", bufs=2))
            for w in range(NW):
                oldw = upt.tile([64, 9, 128], fp32, tag="oldw")
                nc.sync.dma_start(
                    oldw[:], oldT_d[:, :].rearrange(
                        "p (q n) -> p q n", q=9)[:, :, w * 128:(w + 1) * 128])
                neww = upt.tile([64, 9, 128], fp32, tag="neww")
                y_ps = y_pool.tile([64, 9 * 128], fp32, tag="yps")
                for kg in range(9):
                    l = 0 if kg == 0 else (1 if kg <= 3 else 2)
                    nc.tensor.matmul(
                        y_ps[:, kg * 128:(kg + 1) * 128],
                        lin_sb[l][:],
                        agg_sb[:, w, kg % 2, kg // 2, :],
                        start=kg in (0, 4, 8), stop=True,
                        skip_group_check=True)
                y0g = upt.tile([C, 128], fp32, tag="y0g")
                nc.scalar.activation(y0g[:], y_ps[:, 0:128], AF.Sigmoid)
                y0s = upt.tile([C, 128], fp32, tag="y0s")
                nc.vector.tensor_tensor(out=y0s[:], in0=y_ps[:, 0:128],
                                        in1=y0g[:], op=ALU.mult)
                nc.vector.tensor_tensor(out=neww[:, 0, :], in0=y0s[:],
                                        in1=oldw[:, 0, :], op=ALU.add)
                g_ps = g_pool.tile([C, 2, 128], fp32, tag="gps")
                for l in (1, 2):
                    nc.tensor.matmul(g_ps[:, l - 1, :], gw_sb[l - 1][:],
                                     neww[:, 0, :], start=(l == 1), stop=True,
                                     skip_group_check=True)
                gts = upt.tile([C, 2, 128], fp32, tag="gts")
                for l in (1, 2):
                    nc.scalar.activation(gts[:, l - 1, :], g_ps[:, l - 1, :],
                                         AF.Sigmoid, bias=gb_sb[l - 1][:, 0:1])
                gy = upt.tile([C, 8, 128], fp32, tag="gy")
                nc.vector.tensor_tensor(
                    out=gy[:, 0:3, :],
                    in0=y_ps[:].rearrange("p (q n) -> p q n", q=9)[:, 1:4, :],
                    in1=gts[:, 0:1, :].broadcast_to((C, 3, 128)),
                    op=ALU.mult)
                nc.vector.tensor_tensor(
                    out=gy[:, 3:8, :],
                    in0=y_ps[:].rearrange("p (q n) -> p q n", q=9)[:, 4:9, :],
                    in1=gts[:, 1:2, :].broadcast_to((C, 5, 128)),
                    op=ALU.mult)
                nc.vector.tensor_tensor(out=neww[:, 1:9, :], in0=gy[:],
                                        in1=oldw[:, 1:9, :], op=ALU.add)
                nc.sync.dma_start(
                    newT_d[:, :].rearrange("p (q n) -> p q n",
                                           q=9)[:, :, w * 128:(w + 1) * 128],
                    neww[:])

    nc.compile()
    return nc


def _emit_tp_l2(nc, ALU, xg, w_sb, shq_t, msgs, pp, zjp, zgp):
    """P products, cg-folded zjQ scalings, FMA as adds (+8 ratio stts).

    All on DVE: single in-order queue, no cross-engine ping-pong; GpSimd
    has ~9us fixed cost per elementwise op and ScalarE cannot multiply
    two tensors, so the TP bulk lives here.
    """
    import concourse.mybir as mybir
    bf16 = mybir.dt.bfloat16

    GP_JS = ()          # GpSimd streams ~0.4 elem/cyc and zjG bufs=1
                        # serialized groups -> ~19us DVE stall per group;
                        # cheaper to keep all zjq on DVE (nnz-only now)
    P0 = pp.tile([128, BP, SCHED2_L2[0]["nblk"] * C], bf16, name="P0", tag="P0")
    P12 = pp.tile([128, BP, SCHED2_L2[2]["nblk"] * C], bf16,
                  name="P12", tag="P12")
    zj12 = zjp.tile([128, BP, SCHED2_L2[2]["nblk"] * C], bf16,
                    name="zj12", tag="zj12")
    zjG = {j: zgp.tile([128, BP, SCHED2_L2[2]["nblk"] * C], bf16,
                       name=f"zjG{j}", tag=f"zjG{j}") for j in GP_JS}

    def emit_zjq(eng, dst, P, qoff, segs):
        for (z0, L) in segs:
            eng.tensor_tensor(
                out=dst[:].rearrange("p b (n c) -> p b n c",
                                     c=C)[:, :, z0:z0 + L, :],
                in0=P[:].rearrange("p b (n c) -> p b n c",
                                   c=C)[:, :, z0:z0 + L, :],
                in1=shq_t[:, :, qoff + z0:qoff + z0 + L, None].broadcast_to(
                    (128, BP, L, C)),
                op=ALU.mult)

    def emit_fma(sch, j, zj):
        for (jj, z0, k0, L) in sch["runs_add"]:
            if jj != j:
                continue
            nc.vector.tensor_tensor(
                out=msgs[:, :, k0 * C:(k0 + L) * C],
                in0=zj[:, :, z0 * C:(z0 + L) * C],
                in1=msgs[:, :, k0 * C:(k0 + L) * C],
                op=ALU.add)
        for (jj, z, k, ratio) in sch["runs_stt"]:
            if jj != j:
                continue
            nc.vector.scalar_tensor_tensor(
                out=msgs[:, :, k * C:(k + 1) * C],
                in0=zj[:, :, z * C:(z + 1) * C],
                scalar=float(ratio),
                in1=msgs[:, :, k * C:(k + 1) * C],
                op0=ALU.mult, op1=ALU.add)

    for gi, sch in enumerate(SCHED2_L2):
        nblk = sch["nblk"]
        P = P0 if gi == 0 else P12
        for (p, ig0, ni, z0) in sch["pops"]:
            nc.vector.tensor_tensor(
                out=P[:].rearrange("p b (n c) -> p b n c",
                                   c=C)[:, :, z0:z0 + ni, :],
                in0=xg[:].rearrange("p b (n c) -> p b n c",
                                    c=C)[:, :, ig0:ig0 + ni, :],
                in1=w_sb[:, :, None, p * C:(p + 1) * C].broadcast_to(
                    (128, BP, ni, C)),
                op=ALU.mult)
        if gi > 0:
            # kick GpSimd's share first so it overlaps DVE's other j's
            for (j, qoff, segs) in sch["zjq"]:
                if j in GP_JS:
                    emit_zjq(nc.gpsimd, zjG[j], P, qoff, segs)
        for (j, qoff, segs) in sch["zjq"]:
            if gi > 0 and j in GP_JS:
                continue
            dst = msgs if gi == 0 else zj12
            emit_zjq(nc.vector, dst, P, qoff, segs)
            if gi == 0:
                continue
            emit_fma(sch, j, zj12)
        if gi > 0:
            for j in GP_JS:
                if any(jj == j for (jj, _, _) in sch["zjq"]):
                    emit_fma(sch, j, zjG[j])


def _emit_tp_l1(nc, ALU, xg, w_sb, shB, msgs, pp):
    """msgs[k] = x * w'_k * sh_k ; w' host-expanded+cg-folded to 9 kg blocks."""
    import concourse.mybir as mybir
    bf16 = mybir.dt.bfloat16
    tmp = pp.tile([128, BP, 9, C], bf16, name="l1tmp", tag="l1tmp")
    nc.vector.tensor_tensor(
        out=tmp[:],
        in0=w_sb[:].rearrange("p b (n c) -> p b n c", c=C),
        in1=shB[:],
        op=ALU.mult)
    nc.vector.tensor_tensor(
        out=msgs[:].rearrange("p b (n c) -> p b n c", c=C),
        in0=tmp[:],
        in1=xg[:, :, None, 0:C].broadcast_to((128, BP, 9, C)),
        op=ALU.mult)


# ---------------- host orchestration ----------------
def _chunked_T(feats_own):
    """[NPC, 576] -> kg-blocked transposed [64, 9*NPC]."""
    out = np.empty((64, 9 * NPC), np.float32)
    for kg in range(9):
        out[:, kg * NPC:(kg + 1) * NPC] = feats_own[:, kg * 64:(kg + 1) * 64].T
    return out


def _unchunk_T(newT):
    """[64, 9*NPC] -> [NPC, 576]."""
    out = np.empty((NPC, 576), np.float32)
    for kg in range(9):
        out[:, kg * 64:(kg + 1) * 64] = newT[:, kg * NPC:(kg + 1) * NPC].T
    return out


_CACHE = {}


def _prep(positions, senders, receivers):
    key = (senders.tobytes(), receivers.tobytes(), positions.tobytes())
    if _CACHE.get("key") == key:
        return _CACHE["val"]
    sh_eff, basis = edge_geometry(positions, senders, receivers)
    owner, local, nodes_of, _ = partition_graph(receivers)
    deg_bin = np.zeros(NCORES * NW, np.int64)
    np.add.at(deg_bin, owner[receivers] * NW + local[receivers] // 128, 1)
    tpw = (int(deg_bin.max()) + 127) // 128
    T = NW * tpw
    assert T % BP == 0
    perm = build_core_edges(receivers, owner, local, tpw)

    valid = perm >= 0
    eg = np.where(valid, perm, 0)
    snd = np.where(valid, senders[eg], 0).astype(np.int16)      # [NC, T*128]
    shp_e = sh_eff[eg] * valid[..., None]                        # [NC, T*128, 9]
    bas_e = basis[eg] * valid[..., None]                         # [NC, T*128, 8]
    lr = np.where(valid, local[receivers[eg]], 0)

    NG = T // BP
    inv = np.float32(1.0 / np.sqrt(AVG_NN))
    sidx = np.empty((NCORES, 128, T * 128 // 16), np.int16)
    shp_h = np.empty((NCORES, 128, T, 9), BF16)
    shq_h = np.empty((NCORES, 128, T, NSHQ), BF16)
    jcols = np.array([j for (j, cg) in SHQ_COLS])
    cgv = np.array([cg for (j, cg) in SHQ_COLS], np.float32)
    bas_h = np.empty((NCORES, NG, 8, BP * 128), BF16)
    smat_h = np.zeros((NCORES, 128, T, 128), BF16)
    for k in range(NCORES):
        s = snd[k].reshape(T * 8, 16)
        sidx[k] = np.tile(s.T, (8, 1))
        shp_f = shp_e[k].reshape(T, 128, 9).transpose(1, 0, 2)
        shp_h[k] = shp_f.astype(BF16)
        shq_h[k] = (shp_f[:, :, jcols] * cgv[None, None, :]).astype(BF16)
        bas_h[k] = bas_e[k].reshape(NG, BP * 128, 8).transpose(0, 2, 1).astype(BF16)
        v = valid[k]
        e_slots = np.arange(T * 128)
        p_, t_ = e_slots % 128, e_slots // 128
        cols = lr[k] - (t_ // tpw) * 128
        ok = v & (cols >= 0) & (cols < 128)
        sm = np.zeros((128, T, 128), np.float32)
        sm[p_[ok], t_[ok], cols[ok]] = inv
        smat_h[k] = sm.astype(BF16)
    val = dict(T=T, NG=NG, tpw=tpw, nodes_of=nodes_of, sidx=sidx,
               shp_h=shp_h, bas_h=bas_h, smat_h=smat_h, shq_h=shq_h)
    _CACHE["key"], _CACHE["val"] = key, val
    return val


EXEC_NS = []


def _run_layer(nc, pre, ftab, oldT_by_core, lw):
    import os
    from concourse.bass_utils import run_bass_kernel_spmd
    in_maps = []
    for k in range(NCORES):
        m = dict(ftab=ftab,
                 sidx=pre["sidx"][k],
                 shp=pre["shp_h"][k],
                 shq=pre["shq_h"][k],
                 basisT=pre["bas_h"][k],
                 smat=pre["smat_h"][k],
                 oldT=oldT_by_core[k],
                 w1=lw["w1"], b1=lw["b1"], w2=lw["w2"], b2=lw["b2"],
                 w3=lw["w3"], lin0=lw["lin"][0], lin1=lw["lin"][1],
                 lin2=lw["lin"][2], gw0=lw["gw"][0], gw1=lw["gw"][1],
                 gb0=lw["gb"][0], gb1=lw["gb"][1])
        in_maps.append(m)
    trace = bool(os.environ.get("KERNEL_TRACE"))
    res = run_bass_kernel_spmd(nc, in_maps, list(range(NCORES)), trace=trace,
                               trace_cores=list(range(NCORES)) if trace else None)
    if trace and res.exec_time_ns is not None:
        EXEC_NS.append(res.exec_time_ns)
    return [res.results[k]["newT"] for k in range(NCORES)]


def _layer_weights(inputs, i, layer2):
    f32 = np.float32
    w3 = np.ascontiguousarray(inputs["mlp_w3"][i], f32)
    if layer2:
        w3p = w3.astype(BF16)
    else:
        w3p = np.empty((H, 576), BF16)
        for kg in range(9):
            p = L1_PATH_OF_K[kg]
            w3p[:, kg * C:(kg + 1) * C] = (
                w3[:, p * C:(p + 1) * C] * L1_CG_OF_K[kg]).astype(BF16)
    return dict(
        w1=np.ascontiguousarray(inputs["mlp_w1"][i], f32).astype(BF16),
        b1=np.ascontiguousarray(inputs["mlp_b1"][i], f32).reshape(H, 1),
        w2=np.ascontiguousarray(inputs["mlp_w2"][i], f32).astype(BF16),
        b2=np.ascontiguousarray(inputs["mlp_b2"][i], f32).reshape(H, 1),
        w3=w3p,
        lin=[np.ascontiguousarray(inputs["lin_self"][i, l], f32)
             for l in range(3)],
        gw=[np.ascontiguousarray(inputs["gate_w"][i, l], f32)
            for l in range(2)],
        gb=[np.ascontiguousarray(inputs["gate_b"][i, l], f32).reshape(C, 1)
            for l in range(2)],
    )


_KERNEL_CACHE = {}


def _get_kernels(T):
    if T not in _KERNEL_CACHE:
        _KERNEL_CACHE[T] = (build_layer_kernel(False, T),
                            build_layer_kernel(True, T))
    return _KERNEL_CACHE[T]


def _pack_ftab(table, ncols):
    out = np.zeros((N_NODES, ncols), BF16)
    used = min(ncols, table.shape[1])
    out[:, :used] = table[:, :used].astype(BF16)
    return out


def kernel(**inputs):
    positions = np.asarray(inputs["positions"], np.float32)
    species = np.asarray(inputs["species"]).astype(np.int64)
    senders = np.asarray(inputs["senders"]).astype(np.int64)
    receivers = np.asarray(inputs["receivers"]).astype(np.int64)

    pre = _prep(positions, senders, receivers)
    T = pre["T"]
    nc1, nc2 = _get_kernels(T)
    nodes_of = pre["nodes_of"]

    # initial features: x0 from species embedding (host; tiny)
    x0 = (np.asarray(inputs["embed"], np.float32)[species]
          @ np.asarray(inputs["w_proj"], np.float32))          # [N, 64]
    table = np.zeros((N_NODES, F), np.float32)
    table[:, 0:C] = x0

    # ---- layer 1 ----
    oldT = [_chunked_T(table[nodes_of[k]]) for k in range(NCORES)]
    lw = _layer_weights(inputs, 0, False)
    newT = _run_layer(nc1, pre, _pack_ftab(table, GCOLP_L1), oldT, lw)

    table2 = np.empty((N_NODES, F), np.float32)
    for k in range(NCORES):
        table2[nodes_of[k]] = _unchunk_T(newT[k])

    # ---- layer 2 ----
    lw = _layer_weights(inputs, 1, True)
    newT2 = _run_layer(nc2, pre, _pack_ftab(table2, GCOLP_L2), newT, lw)

    table3 = np.empty((N_NODES, F), np.float32)
    for k in range(NCORES):
        table3[nodes_of[k]] = _unchunk_T(newT2[k])

    # ---- output: reorder component-major -> reference layout + alpha ----
    t3 = table3.reshape(N_NODES, 9, C)
    out = np.empty((N_NODES, F), np.float32)
    out[:, 0:64] = t3[:, 0]
    out[:, 64:256] = (0.5 * t3[:, 1:4]).transpose(0, 2, 1).reshape(N_NODES, 192)
    out[:, 256:576] = (0.25 * t3[:, 4:9]).transpose(0, 2, 1).reshape(N_NODES, 320)
    return out



# revision 21
# speedup vs baseline: 1.3171x; 1.1237x over previous
"""NequIP GNN message-passing kernel for 8 Trainium2 NeuronCores — v2.

Receiver-sharded graph parallelism (per sharding hint): host LPT-assigns the
8192 nodes to 64 (core, window) bins of 128, each core owns 8 windows = 1024
nodes + their in-edges, sorted by window, padded to 128-edge tiles.

v2 device pipeline (vs v1): bf16 edge pipeline end-to-end with fp32 PSUM
accumulation; per-edge-scalar stages batched across 8-tile groups via
stride-0 broadcast access patterns; P-products collapsed per-path (w operand
broadcast over l1 components); CG-FMA stage merged into diagonal runs with
immediate scalars; segment-sum via paired-kg bf16 matmuls; radial MLP with
fused Silu activations; elementwise work split across Vector/GpSimd/Scalar.
"""
import math
import numpy as np
import ml_dtypes

BF16 = ml_dtypes.bfloat16

# ---------------- model constants ----------------
N_NODES, N_EDGES = 8192, 131072
C, H, NRAD = 64, 64, 8
R_MAX, AVG_NN = 5.0, 16.0
NCORES, NPC = 8, 1024
NW = NPC // 128
F = 9 * C
LS = (0, 1, 2)
PATHS = [(l1, l2, l3) for l1 in LS for l2 in LS for l3 in LS
         if abs(l1 - l2) <= l3 <= l1 + l2]
LOFF = {0: 0, 1: 1, 2: 4}
J_OF_L2 = {0: [0], 1: [1, 2, 3], 2: [4, 5, 6, 7, 8]}
BP = 8                      # tiles per group (batched in op free dims)
GCOLP_L2 = 640              # padded gather row (bf16): 1280B % 256 == 0
GCOLP_L1 = 128              # 256B % 256 == 0


# ---------------- real Clebsch-Gordan coefficients ----------------
def _cg_scalar(j1, m1, j2, m2, j3, m3):
    f = math.factorial
    if m1 + m2 != m3:
        return 0.0
    pre = ((2*j3+1) * f(j1+j2-j3) * f(j1-j2+j3) * f(-j1+j2+j3)
           / f(j1+j2+j3+1)) ** 0.5
    pre *= (f(j1+m1)*f(j1-m1)*f(j2+m2)*f(j2-m2)*f(j3+m3)*f(j3-m3)) ** 0.5
    s = 0.0
    for k in range(max(0, j2-j3-m1, j1+m2-j3), min(j1+j2-j3, j1-m1, j2+m2)+1):
        s += (-1)**k / (f(k)*f(j1+j2-j3-k)*f(j1-m1-k)
                        * f(j2+m2-k)*f(j3-j2+m1+k)*f(j3-j1-m2+k))
    return pre * s


def _U_real(l):
    U = np.zeros((2*l+1, 2*l+1), dtype=complex)
    s2 = 2 ** -0.5
    for m in range(-l, l+1):
        if m > 0:
            U[m+l, m+l] = (-1)**m * s2
            U[m+l, -m+l] = s2
        elif m == 0:
            U[l, l] = 1.0
        else:
            U[m+l, m+l] = 1j*s2
            U[m+l, -m+l] = -1j*(-1)**(-m)*s2
    return U


def _real_cg(l1, l2, l3):
    Cc = np.zeros((2*l1+1, 2*l2+1, 2*l3+1))
    for i1, m1 in enumerate(range(-l1, l1+1)):
        for i2, m2 in enumerate(range(-l2, l2+1)):
            m3 = m1 + m2
            if abs(m3) <= l3:
                Cc[i1, i2, m3+l3] = _cg_scalar(l1, m1, l2, m2, l3, m3)
    U1, U2, U3 = _U_real(l1), _U_real(l2), _U_real(l3)
    W = np.einsum('ia,jb,kc,abc->ijk', U1.conj(), U2.conj(), U3,
                  Cc.astype(complex))
    W = W.real if np.linalg.norm(W.real) >= np.linalg.norm(W.imag) else W.imag
    W = W / np.linalg.norm(W) * (2*l3+1) ** 0.5
    return np.asarray(W, dtype=np.float64)


CGS = [_real_cg(*p) for p in PATHS]


def build_schedule_l2():
    """Static TP structure for layer 2, grouped by l2.

    Per group: nblk, pops=[(path, ig0, ni, z0)], jlist, runs=[(j,z0,k0,L,cg)].
    """
    scheds = []
    for l2 in (0, 1, 2):
        ps = [p for p in range(len(PATHS)) if PATHS[p][1] == l2]
        blocks, block_of, pops = [], {}, []
        for p in ps:
            l1 = PATHS[p][0]
            ni = 2 * l1 + 1
            pops.append((p, LOFF[l1], ni, len(blocks)))
            for i in range(ni):
                block_of[(p, i)] = len(blocks)
                blocks.append((p, i))
        tset = set()
        for p in ps:
            l1, _, l3 = PATHS[p]
            cg = CGS[p]
            for i in range(2*l1+1):
                for j in range(2*l2+1):
                    for k in range(2*l3+1):
                        v = cg[i, j, k]
                        if abs(v) > 1e-12:
                            tset.add((LOFF[l2]+j, block_of[(p, i)],
                                      LOFF[l3]+k, round(float(v), 9)))
        runs, consumed = [], set()
        for t in sorted(tset):
            if t in consumed:
                continue
            j, z, k, cgv = t
            L = 0
            while (j, z+L, k+L, cgv) in tset and (j, z+L, k+L, cgv) not in consumed:
                consumed.add((j, z+L, k+L, cgv))
                L += 1
            runs.append((j, z, k, L, cgv))
        scheds.append(dict(l2=l2, nblk=len(blocks), pops=pops,
                           jlist=J_OF_L2[l2], runs=runs))
    return scheds


SCHED_L2 = build_schedule_l2()


def build_schedule2():
    """cg folded into per-(j,z) host scalars; FMA mostly tensor adds.

    Per group: nblk, pops, zjq=[(j, qoff)], runs_add=[(j,z0,k0,L)],
    runs_stt=[(j,z,k,ratio)]. qoff = column offset of (j,*) block in the
    concatenated shq table [128, T, 179]. cgfold[qoff+z] = cg of the primary
    (smallest-k) triple of (j,z); 0 for unused (z,j).
    """
    scheds, qoff, cgcols = [], 0, []
    for gi, sch in enumerate(SCHED_L2):
        nblk = sch["nblk"]
        tset = set()
        for (j, z, k, L, cg) in sch["runs"]:
            for i in range(L):
                tset.add((j, z + i, k + i, cg))
        per_jz = {}
        for (j, z, k, cg) in tset:
            per_jz.setdefault((j, z), []).append((k, cg))
        primary, runs_stt = set(), []
        cgf = {}
        for (j, z), ks in per_jz.items():
            ks.sort()
            k0, cg0 = ks[0]
            primary.add((j, z, k0))
            cgf[(j, z)] = cg0
            for (k1, cg1) in ks[1:]:
                runs_stt.append((j, z, k1, cg1 / cg0))
        runs_add, consumed = [], set()
        for t in sorted(primary):
            if t in consumed:
                continue
            j, z, k = t
            L = 0
            while (j, z + L, k + L) in primary and (j, z + L, k + L) not in consumed:
                consumed.add((j, z + L, k + L))
                L += 1
            runs_add.append((j, z, k, L))
        zjq = []
        for j in sch["jlist"]:
            # contiguous segments of z columns actually used by (j, z):
            # skips the ~28% of dense columns with cg == 0
            used = sorted(z for (jj, z) in cgf if jj == j)
            segs, s0, prev = [], None, None
            for z in used:
                if s0 is None:
                    s0 = prev = z
                elif z == prev + 1:
                    prev = z
                else:
                    segs.append((s0, prev - s0 + 1))
                    s0 = prev = z
            if s0 is not None:
                segs.append((s0, prev - s0 + 1))
            zjq.append((j, qoff, segs))
            for z in range(nblk):
                cgcols.append((j, cgf.get((j, z), 0.0)))
            qoff += nblk
        scheds.append(dict(nblk=nblk, pops=sch["pops"], zjq=zjq,
                           runs_add=runs_add, runs_stt=runs_stt))
    return scheds, cgcols


SCHED2_L2, SHQ_COLS = build_schedule2()
NSHQ = len(SHQ_COLS)

# layer-1 per-k path and cg (x is scalar-only: paths (0,l,l), j == k)
L1_PATH_OF_K = [0] + [1]*3 + [2]*5
L1_CG_OF_K = []
for _k in range(9):
    _p = L1_PATH_OF_K[_k]
    _l = PATHS[_p][2]
    _m = _k - LOFF[_l]
    L1_CG_OF_K.append(float(CGS[_p][0, _m, _m]))


# ---------------- host-side graph preprocessing ----------------
def edge_geometry(positions, senders, receivers):
    rel = (positions[receivers] - positions[senders]) / R_MAX
    d = np.linalg.norm(rel, axis=-1)
    u = rel / np.maximum(d, 1e-6)[:, None]
    x, y, z = u[:, 0], u[:, 1], u[:, 2]
    sh = np.empty((len(d), 9), np.float32)
    sh[:, 0] = 1.0
    sh[:, 1] = np.sqrt(3.0) * y
    sh[:, 2] = np.sqrt(3.0) * z
    sh[:, 3] = np.sqrt(3.0) * x
    sh[:, 4] = np.sqrt(15.0) * x * y
    sh[:, 5] = np.sqrt(15.0) * y * z
    sh[:, 6] = np.sqrt(5.0) / 2 * (3 * z * z - 1.0)
    sh[:, 7] = np.sqrt(15.0) * x * z
    sh[:, 8] = np.sqrt(15.0) / 2 * (x * x - y * y)
    freqs = np.arange(1, NRAD + 1, dtype=np.float64)
    xr = np.clip(d, 1e-4, 1.0)[:, None].astype(np.float64)
    basis = (np.sqrt(2.0) * np.sin(freqs * np.pi * xr) / xr).astype(np.float32)
    cut = (0.5 * (np.cos(np.pi * np.clip(d, 0.0, 1.0)) + 1.0)).astype(np.float32)
    return (sh * cut[:, None]).astype(np.float32), basis


def partition_graph(receivers):
    import heapq
    deg = np.bincount(receivers, minlength=N_NODES)
    order = np.argsort(-deg, kind="stable")
    nbins = NCORES * NW
    load = np.zeros(nbins, np.int64)
    cnt = np.zeros(nbins, np.int64)
    owner = np.empty(N_NODES, np.int32)
    local = np.empty(N_NODES, np.int32)
    heap = [(0, b) for b in range(nbins)]
    heapq.heapify(heap)
    for n in order:
        while True:
            l, b = heapq.heappop(heap)
            if cnt[b] < 128:
                break
        owner[n] = b // NW
        local[n] = (b % NW) * 128 + cnt[b]
        cnt[b] += 1
        load[b] += deg[n]
        if cnt[b] < 128:
            heapq.heappush(heap, (int(load[b]), b))
    nodes_of = np.empty((NCORES, NPC), np.int64)
    for n in range(N_NODES):
        nodes_of[owner[n], local[n]] = n
    return owner, local, nodes_of, int(load.max())


def build_core_edges(receivers, owner, local, tpw):
    T = NW * tpw
    perm = np.full((NCORES, T * 128), -1, np.int64)
    for k in range(NCORES):
        eids = np.where(owner[receivers] == k)[0]
        lr = local[receivers[eids]]
        o = np.argsort(lr, kind="stable")
        eids, lr = eids[o], lr[o]
        w_of = lr // 128
        for w in range(NW):
            sel = eids[w_of == w]
            assert len(sel) <= tpw * 128, "tiles-per-window overflow"
            base = w * tpw * 128
            perm[k, base:base + len(sel)] = sel
    return perm


# ---------------- bass kernel builder ----------------
def build_layer_kernel(layer2, T, debug=False):
    import concourse.bass as bass
    import concourse.bacc as bacc
    import concourse.tile as tile
    import concourse.mybir as mybir
    from contextlib import ExitStack

    fp32 = mybir.dt.float32
    bf16 = mybir.dt.bfloat16
    AF = mybir.ActivationFunctionType
    ALU = mybir.AluOpType

    NPATH = 15 if layer2 else 3
    GCOLP = GCOLP_L2 if layer2 else GCOLP_L1
    W3COL = NPATH * C if layer2 else 576   # L1 w3 host-expanded to 9 kg blocks
    E_PAD = T * 128
    NG = T // BP
    assert T % BP == 0 and T % NW == 0
    tpw = T // NW

    import os as _os
    STAGE = int(_os.environ.get("KV2_STAGE", "5"))
    nc = bacc.Bacc("TRN2", target_bir_lowering=False)

    ftab = nc.dram_tensor("ftab", [N_NODES, GCOLP], bf16, kind="ExternalInput")
    sidx = nc.dram_tensor("sidx", [128, E_PAD // 16], mybir.dt.int16,
                          kind="ExternalInput")
    shp_d = nc.dram_tensor("shp", [128, T, 9], bf16, kind="ExternalInput")
    shq_d = nc.dram_tensor("shq", [128, T, NSHQ], bf16, kind="ExternalInput")
    basT_d = nc.dram_tensor("basisT", [NG, 8, BP * 128], bf16,
                            kind="ExternalInput")
    smat_d = nc.dram_tensor("smat", [128, T, 128], bf16, kind="ExternalInput")
    oldT_d = nc.dram_tensor("oldT", [64, 9 * NPC], fp32, kind="ExternalInput")
    w1_d = nc.dram_tensor("w1", [8, H], bf16, kind="ExternalInput")
    b1_d = nc.dram_tensor("b1", [H, 1], fp32, kind="ExternalInput")
    w2_d = nc.dram_tensor("w2", [H, H], bf16, kind="ExternalInput")
    b2_d = nc.dram_tensor("b2", [H, 1], fp32, kind="ExternalInput")
    w3_d = nc.dram_tensor("w3", [H, W3COL], bf16, kind="ExternalInput")
    lin_d = [nc.dram_tensor(f"lin{l}", [C, C], fp32, kind="ExternalInput")
             for l in range(3)]
    gw_d = [nc.dram_tensor(f"gw{l}", [C, C], fp32, kind="ExternalInput")
            for l in range(2)]
    gb_d = [nc.dram_tensor(f"gb{l}", [C, 1], fp32, kind="ExternalInput")
            for l in range(2)]
    newT_d = nc.dram_tensor("newT", [64, 9 * NPC], fp32,
                            kind="ExternalOutput")
    if debug:
        dbg_xg = nc.dram_tensor("dbg_xg", [128, BP, GCOLP], fp32,
                                kind="ExternalOutput")
        dbg_w = nc.dram_tensor("dbg_w", [128, BP, W3COL], fp32,
                               kind="ExternalOutput")
        dbg_msgs = nc.dram_tensor("dbg_msgs", [128, BP, F], fp32,
                                  kind="ExternalOutput")
        dbg_agg = nc.dram_tensor("dbg_agg", [128, NW, 640], fp32,
                                 kind="ExternalOutput")

    with tile.TileContext(nc) as tc, ExitStack() as ctx:
        consts = ctx.enter_context(tc.tile_pool(name="consts", bufs=1))
        idx_sb = consts.tile([128, E_PAD // 16], mybir.dt.int16)
        nc.sync.dma_start(idx_sb[:], sidx[:])
        w1_sb = consts.tile([8, H], bf16)
        nc.sync.dma_start(w1_sb[:], w1_d[:])
        b1_sb = consts.tile([H, 1], fp32)
        nc.sync.dma_start(b1_sb[:], b1_d[:])
        w2_sb = consts.tile([H, H], bf16)
        nc.sync.dma_start(w2_sb[:], w2_d[:])
        b2_sb = consts.tile([H, 1], fp32)
        nc.sync.dma_start(b2_sb[:], b2_d[:])
        w3_sb = consts.tile([H, W3COL], bf16)
        nc.sync.dma_start(w3_sb[:], w3_d[:])
        lin_sb = [consts.tile([C, C], fp32, name=f"lin{l}", tag=f"lin{l}")
                  for l in range(3)]
        for l in range(3):
            nc.sync.dma_start(lin_sb[l][:], lin_d[l][:])
        gw_sb = [consts.tile([C, C], fp32, name=f"gw{l}", tag=f"gw{l}")
                 for l in range(2)]
        gb_sb = [consts.tile([C, 1], fp32, name=f"gb{l}", tag=f"gb{l}")
                 for l in range(2)]
        for l in range(2):
            nc.sync.dma_start(gw_sb[l][:], gw_d[l][:])
            nc.sync.dma_start(gb_sb[l][:], gb_d[l][:])
        agg_sb = consts.tile([64, NW, 2, 5, 128], fp32)

        with ExitStack() as psctx:
            iop = psctx.enter_context(tc.tile_pool(name="iop", bufs=2))
            aggt = psctx.enter_context(tc.tile_pool(name="aggt", bufs=1))
            wp = psctx.enter_context(tc.tile_pool(name="wp", bufs=1))
            msgp = psctx.enter_context(tc.tile_pool(name="msgp", bufs=2))
            shbp = psctx.enter_context(tc.tile_pool(name="shbp", bufs=1))
            pp = psctx.enter_context(tc.tile_pool(name="pp", bufs=1))
            zjp = psctx.enter_context(tc.tile_pool(name="zjp", bufs=1))
            zgp = psctx.enter_context(tc.tile_pool(name="zgp", bufs=1))
            h_ps = psctx.enter_context(
                tc.tile_pool(name="h_ps", bufs=1, space="PSUM"))
            w_ps_pool = psctx.enter_context(
                tc.tile_pool(name="w_ps", bufs=1, space="PSUM"))
            agg_pool = psctx.enter_context(
                tc.tile_pool(name="agg_ps", bufs=2, space="PSUM"))

            agg_open = {}

            for g in range(NG):
                t0 = g * BP
                xg = iop.tile([128, BP, GCOLP], bf16, tag="xg")
                nc.gpsimd.dma_gather(
                    out_ap=xg[:],
                    in_ap=ftab[:],
                    idxs_ap=idx_sb[:, g * (BP * 8):(g + 1) * (BP * 8)],
                    num_idxs=BP * 128,
                    num_idxs_reg=BP * 128,
                    elem_size=GCOLP,
                )
                shp_t = iop.tile([128, BP, 9], bf16, tag="shp")
                nc.sync.dma_start(shp_t[:], shp_d[:, t0:t0 + BP, :])
                if layer2:
                    shq_t = iop.tile([128, BP, NSHQ], bf16, tag="shq")
                    nc.sync.dma_start(shq_t[:], shq_d[:, t0:t0 + BP, :])
                smt = iop.tile([128, BP, 128], bf16, tag="smt")
                nc.sync.dma_start(smt[:], smat_d[:, t0:t0 + BP, :])
                bas = iop.tile([8, BP * 128], bf16, tag="bas")
                nc.sync.dma_start(bas[:], basT_d[g, :, :])

                if STAGE < 2:
                    continue
                if not layer2:
                    # sh broadcast table for L1 (ScalarE, stride-0 copy)
                    shB = shbp.tile([128, BP, 9, C], bf16, tag="shB")
                    nc.scalar.copy(
                        shB[:],
                        shp_t[:, :, :, None].broadcast_to((128, BP, 9, C)))

                # ---- radial MLP (transposed; fused Silu) ----
                h1s = iop.tile([H, BP * 128], bf16, tag="h1s")
                h2s = iop.tile([H, BP * 128], bf16, tag="h2s")
                sg = iop.tile([H, BP * 128], bf16, tag="sg")
                for c0 in range(0, BP * 128, 512):
                    h1p = h_ps.tile([H, 512], fp32, tag="h")
                    nc.tensor.matmul(h1p[:], w1_sb[:],
                                     bas[:, c0:c0 + 512], start=True, stop=True)
                    nc.scalar.activation(sg[:, c0:c0 + 512], h1p[:],
                                         AF.Sigmoid, bias=b1_sb[:, 0:1])
                    nc.vector.scalar_tensor_tensor(
                        out=h1s[:, c0:c0 + 512], in0=h1p[:],
                        scalar=b1_sb[:, 0:1], in1=sg[:, c0:c0 + 512],
                        op0=ALU.add, op1=ALU.mult)
                for c0 in range(0, BP * 128, 512):
                    h2p = h_ps.tile([H, 512], fp32, tag="h")
                    nc.tensor.matmul(h2p[:], w2_sb[:],
                                     h1s[:, c0:c0 + 512], start=True, stop=True)
                    nc.scalar.activation(sg[:, c0:c0 + 512], h2p[:],
                                         AF.Sigmoid, bias=b2_sb[:, 0:1])
                    nc.vector.scalar_tensor_tensor(
                        out=h2s[:, c0:c0 + 512], in0=h2p[:],
                        scalar=b2_sb[:, 0:1], in1=sg[:, c0:c0 + 512],
                        op0=ALU.add, op1=ALU.mult)

                # ---- per-tile edge weights w = h2s_t.T @ W3 (bf16 out) ----
                w_sb = wp.tile([128, BP, W3COL], bf16, tag="wsb")
                for bt in range(BP):
                    w_ps = w_ps_pool.tile([128, W3COL], fp32, tag="wps")
                    for c0 in range(0, W3COL, 512):
                        c1 = min(c0 + 512, W3COL)
                        nc.tensor.matmul(w_ps[:, c0:c1],
                                         h2s[:, bt * 128:(bt + 1) * 128],
                                         w3_sb[:, c0:c1], start=True, stop=True)
                    nc.scalar.copy(w_sb[:, bt, :], w_ps[:, 0:W3COL])

                if STAGE < 3:
                    continue
                # ---- tensor product ----
                msgs = msgp.tile([128, BP, F], bf16, tag="msgs")
                if layer2:
                    _emit_tp_l2(nc, ALU, xg, w_sb, shq_t, msgs, pp, zjp, zgp)
                else:
                    _emit_tp_l1(nc, ALU, xg, w_sb, shB, msgs, pp)

                if debug and g == 0:
                    dxg = pp.tile([128, BP, GCOLP], fp32, tag="dxg")
                    nc.vector.tensor_copy(out=dxg[:], in_=xg[:])
                    nc.sync.dma_start(dbg_xg[:], dxg[:])
                    dw = pp.tile([128, BP, W3COL], fp32, tag="dw")
                    nc.vector.tensor_copy(out=dw[:], in_=w_sb[:])
                    nc.sync.dma_start(dbg_w[:], dw[:])
                    dmg = pp.tile([128, BP, F], fp32, tag="dmg")
                    nc.vector.tensor_copy(out=dmg[:], in_=msgs[:])
                    nc.sync.dma_start(dbg_msgs[:], dmg[:])

                if STAGE < 4:
                    continue
                # ---- segment sum: paired-kg bf16 matmuls, PSUM-accumulated ----
                for bt in range(BP):
                    t = t0 + bt
                    w, t_in_w = t // tpw, t % tpw
                    if w not in agg_open:
                        agg_open[w] = agg_pool.tile([128, 640], fp32,
                                                    name="aggps", tag="aggps")
                    ps = agg_open[w]
                    first, last = t_in_w == 0, t_in_w == tpw - 1
                    for pair in range(5):
                        c0 = pair * 128
                        cw = 128 if pair < 4 else 64
                        nc.tensor.matmul(
                            ps[0:cw, c0:c0 + 128],
                            msgs[:, bt, c0:c0 + cw],
                            smt[:, bt, :],
                            start=first and pair in (0, 4),
                            stop=last, skip_group_check=True)
                    if last:
                        aps = agg_open.pop(w)
                        afp = aggt.tile([128, 640], fp32, tag="afp")
                        nc.scalar.copy(afp[:, 0:512], aps[:, 0:512])
                        nc.scalar.copy(afp[0:64, 512:640], aps[0:64, 512:640])
                        nc.sync.dma_start(
                            agg_sb[:, w, 0, :, :],
                            afp[0:64, :].rearrange("p (q n) -> p q n", q=5))
                        nc.sync.dma_start(
                            agg_sb[:, w, 1, 0:4, :],
                            afp[64:128, 0:512].rearrange("p (q n) -> p q n",
                                                         q=4))



        # ---------------- per-window node update ----------------
        if STAGE < 5:
            with ExitStack() as upctx:
                upt = upctx.enter_context(tc.tile_pool(name="upt", bufs=2))
                for w in range(NW):
                    tmp = upt.tile([64, 9, 128], fp32, tag="pass")
                    nc.sync.dma_start(
                        tmp[:], oldT_d[:, :].rearrange(
                            "p (q n) -> p q n",
                            q=9)[:, :, w * 128:(w + 1) * 128])
                    nc.sync.dma_start(
                        newT_d[:, :].rearrange(
                            "p (q n) -> p q n",
                            q=9)[:, :, w * 128:(w + 1) * 128],
                        tmp[:])
            nc.compile()
            return nc
        with ExitStack() as upctx:
            y_pool = upctx.enter_context(
                tc.tile_pool(name="y_ps", bufs=2, space="PSUM"))
            g_pool = upctx.enter_context(
                tc.tile_pool(name="g_ps", bufs=2, space="PSUM"))
            upt = upctx.enter_context(tc.tile_pool(name="upt", bufs=2))
            for w in range(NW):
                oldw = upt.tile([64, 9, 128], fp32, tag="oldw")
                nc.sync.dma_start(
                    oldw[:], oldT_d[:, :].rearrange(
                        "p (q n) -> p q n", q=9)[:, :, w * 128:(w + 1) * 128])
                neww = upt.tile([64, 9, 128], fp32, tag="neww")
                y_ps = y_pool.tile([64, 9 * 128], fp32, tag="yps")
                for kg in range(9):
                    l = 0 if kg == 0 else (1 if kg <= 3 else 2)
                    nc.tensor.matmul(
                        y_ps[:, kg * 128:(kg + 1) * 128],
                        lin_sb[l][:],
                        agg_sb[:, w, kg % 2, kg // 2, :],
                        start=kg in (0, 4, 8), stop=True,
                        skip_group_check=True)
                y0g = upt.tile([C, 128], fp32, tag="y0g")
                nc.scalar.activation(y0g[:], y_ps[:, 0:128], AF.Sigmoid)
                y0s = upt.tile([C, 128], fp32, tag="y0s")
                nc.vector.tensor_tensor(out=y0s[:], in0=y_ps[:, 0:128],
                                        in1=y0g[:], op=ALU.mult)
                nc.vector.tensor_tensor(out=neww[:, 0, :], in0=y0s[:],
                                        in1=oldw[:, 0, :], op=ALU.add)
                g_ps = g_pool.tile([C, 2, 128], fp32, tag="gps")
                for l in (1, 2):
                    nc.tensor.matmul(g_ps[:, l - 1, :], gw_sb[l - 1][:],
                                     neww[:, 0, :], start=(l == 1), stop=True,
                                     skip_group_check=True)
                gts = upt.tile([C, 2, 128], fp32, tag="gts")
                for l in (1, 2):
                    nc.scalar.activation(gts[:, l - 1, :], g_ps[:, l - 1, :],
                                         AF.Sigmoid, bias=gb_sb[l - 1][:, 0:1])
                gy = upt.tile([C, 8, 128], fp32, tag="gy")
                nc.vector.tensor_tensor(
                    out=gy[:, 0:3, :],
                    in0=y_ps[:].rearrange("p (q n) -> p q n", q=9)[:, 1:4, :],
                    in1=gts[:, 0:1, :].broadcast_to((C, 3, 128)),
                    op=ALU.mult)
                nc.vector.tensor_tensor(
                    out=gy[:, 3:8, :],
                    in0=y_ps[:].rearrange("p (q n) -> p q n", q=9)[:, 4:9, :],
                    in1=gts[:, 1:2, :].broadcast_to((C, 5, 128)),
                    op=ALU.mult)
                nc.vector.tensor_tensor(out=neww[:, 1:9, :], in0=gy[:],
                                        in1=oldw[:, 1:9, :], op=ALU.add)
                nc.sync.dma_start(
                    newT_d[:, :].rearrange("p (q n) -> p q n",
                                           q=9)[:, :, w * 128:(w + 1) * 128],
                    neww[:])

    nc.compile()
    return nc


def _emit_tp_l2(nc, ALU, xg, w_sb, shq_t, msgs, pp, zjp, zgp):
    """P products, cg-folded zjQ scalings, FMA as adds (+8 ratio stts).

    All on DVE: single in-order queue, no cross-engine ping-pong; GpSimd
    has ~9us fixed cost per elementwise op and ScalarE cannot multiply
    two tensors, so the TP bulk lives here.
    """
    import concourse.mybir as mybir
    bf16 = mybir.dt.bfloat16

    GP_JS = ()          # GpSimd streams ~0.4 elem/cyc and zjG bufs=1
                        # serialized groups -> ~19us DVE stall per group;
                        # cheaper to keep all zjq on DVE (nnz-only now)
    EXPC = 11           # ping-pong expansion chunk: 2 x 8x11x64 bf16 = 22.5KB
    P0 = pp.tile([128, BP, SCHED2_L2[0]["nblk"] * C], bf16, name="P0", tag="P0")
    P12 = pp.tile([128, BP, SCHED2_L2[2]["nblk"] * C], bf16,
                  name="P12", tag="P12")
    zj12 = zjp.tile([128, BP, SCHED2_L2[2]["nblk"] * C], bf16,
                    name="zj12", tag="zj12")
    exps = [zgp.tile([128, BP, EXPC, C], bf16, name=f"exp{i}", tag=f"exp{i}")
            for i in range(2)]
    ppg = [0]

    def emit_zjq(eng, dst, P, qoff, segs):
        # ScalarE pre-broadcasts shq cols to C width (stride-0 src is fine
        # on ACT); DVE then multiplies with unit-stride in1 -> 2x_1P mode
        # (the old stride-0 broadcast in1 forced 1x). Ping-pong chunks of
        # <= EXPC cols so ScalarE expansion overlaps DVE consumption.
        chunks, cur, used = [], [], 0
        for (z0, L) in segs:
            while L > 0:
                if used == EXPC:
                    chunks.append(cur)
                    cur, used = [], 0
                take = min(L, EXPC - used)
                cur.append((z0, take, used))
                z0, L, used = z0 + take, L - take, used + take
        if cur:
            chunks.append(cur)
        for ch in chunks:
            exp = exps[ppg[0]]
            ppg[0] ^= 1
            for (z0, L, eo) in ch:
                nc.scalar.copy(
                    exp[:, :, eo:eo + L, :],
                    shq_t[:, :, qoff + z0:qoff + z0 + L, None].broadcast_to(
                        (128, BP, L, C)))
            for (z0, L, eo) in ch:
                eng.tensor_tensor(
                    out=dst[:].rearrange("p b (n c) -> p b n c",
                                         c=C)[:, :, z0:z0 + L, :],
                    in0=P[:].rearrange("p b (n c) -> p b n c",
                                       c=C)[:, :, z0:z0 + L, :],
                    in1=exp[:, :, eo:eo + L, :],
                    op=ALU.mult)

    def emit_fma(sch, j, zj):
        for (jj, z0, k0, L) in sch["runs_add"]:
            if jj != j:
                continue
            nc.vector.tensor_tensor(
                out=msgs[:, :, k0 * C:(k0 + L) * C],
                in0=zj[:, :, z0 * C:(z0 + L) * C],
                in1=msgs[:, :, k0 * C:(k0 + L) * C],
                op=ALU.add)
        for (jj, z, k, ratio) in sch["runs_stt"]:
            if jj != j:
                continue
            nc.vector.scalar_tensor_tensor(
                out=msgs[:, :, k * C:(k + 1) * C],
                in0=zj[:, :, z * C:(z + 1) * C],
                scalar=float(ratio),
                in1=msgs[:, :, k * C:(k + 1) * C],
                op0=ALU.mult, op1=ALU.add)

    for gi, sch in enumerate(SCHED2_L2):
        nblk = sch["nblk"]
        P = P0 if gi == 0 else P12
        for (p, ig0, ni, z0) in sch["pops"]:
            nc.vector.tensor_tensor(
                out=P[:].rearrange("p b (n c) -> p b n c",
                                   c=C)[:, :, z0:z0 + ni, :],
                in0=xg[:].rearrange("p b (n c) -> p b n c",
                                    c=C)[:, :, ig0:ig0 + ni, :],
                in1=w_sb[:, :, None, p * C:(p + 1) * C].broadcast_to(
                    (128, BP, ni, C)),
                op=ALU.mult)
        if gi > 0:
            # kick GpSimd's share first so it overlaps DVE's other j's
            for (j, qoff, segs) in sch["zjq"]:
                if j in GP_JS:
                    emit_zjq(nc.gpsimd, zjG[j], P, qoff, segs)
        for (j, qoff, segs) in sch["zjq"]:
            if gi > 0 and j in GP_JS:
                continue
            dst = msgs if gi == 0 else zj12
            emit_zjq(nc.vector, dst, P, qoff, segs)
            if gi == 0:
                continue
            emit_fma(sch, j, zj12)
        if gi > 0:
            for j in GP_JS:
                if any(jj == j for (jj, _, _) in sch["zjq"]):
                    emit_fma(sch, j, zjG[j])


def _emit_tp_l1(nc, ALU, xg, w_sb, shB, msgs, pp):
    """msgs[k] = x * w'_k * sh_k ; w' host-expanded+cg-folded to 9 kg blocks."""
    import concourse.mybir as mybir
    bf16 = mybir.dt.bfloat16
    tmp = pp.tile([128, BP, 9, C], bf16, name="l1tmp", tag="l1tmp")
    nc.vector.tensor_tensor(
        out=tmp[:],
        in0=w_sb[:].rearrange("p b (n c) -> p b n c", c=C),
        in1=shB[:],
        op=ALU.mult)
    nc.vector.tensor_tensor(
        out=msgs[:].rearrange("p b (n c) -> p b n c", c=C),
        in0=tmp[:],
        in1=xg[:, :, None, 0:C].broadcast_to((128, BP, 9, C)),
        op=ALU.mult)


# ---------------- host orchestration ----------------
def _chunked_T(feats_own):
    """[NPC, 576] -> kg-blocked transposed [64, 9*NPC]."""
    out = np.empty((64, 9 * NPC), np.float32)
    for kg in range(9):
        out[:, kg * NPC:(kg + 1) * NPC] = feats_own[:, kg * 64:(kg + 1) * 64].T
    return out


def _unchunk_T(newT):
    """[64, 9*NPC] -> [NPC, 576]."""
    out = np.empty((NPC, 576), np.float32)
    for kg in range(9):
        out[:, kg * 64:(kg + 1) * 64] = newT[:, kg * NPC:(kg + 1) * NPC].T
    return out


_CACHE = {}


def _prep(positions, senders, receivers):
    key = (senders.tobytes(), receivers.tobytes(), positions.tobytes())
    if _CACHE.get("key") == key:
        return _CACHE["val"]
    sh_eff, basis = edge_geometry(positions, senders, receivers)
    owner, local, nodes_of, _ = partition_graph(receivers)
    deg_bin = np.zeros(NCORES * NW, np.int64)
    np.add.at(deg_bin, owner[receivers] * NW + local[receivers] // 128, 1)
    tpw = (int(deg_bin.max()) + 127) // 128
    T = NW * tpw
    assert T % BP == 0
    perm = build_core_edges(receivers, owner, local, tpw)

    valid = perm >= 0
    eg = np.where(valid, perm, 0)
    snd = np.where(valid, senders[eg], 0).astype(np.int16)      # [NC, T*128]
    shp_e = sh_eff[eg] * valid[..., None]                        # [NC, T*128, 9]
    bas_e = basis[eg] * valid[..., None]                         # [NC, T*128, 8]
    lr = np.where(valid, local[receivers[eg]], 0)

    NG = T // BP
    inv = np.float32(1.0 / np.sqrt(AVG_NN))
    sidx = np.empty((NCORES, 128, T * 128 // 16), np.int16)
    shp_h = np.empty((NCORES, 128, T, 9), BF16)
    shq_h = np.empty((NCORES, 128, T, NSHQ), BF16)
    jcols = np.array([j for (j, cg) in SHQ_COLS])
    cgv = np.array([cg for (j, cg) in SHQ_COLS], np.float32)
    bas_h = np.empty((NCORES, NG, 8, BP * 128), BF16)
    smat_h = np.zeros((NCORES, 128, T, 128), BF16)
    for k in range(NCORES):
        s = snd[k].reshape(T * 8, 16)
        sidx[k] = np.tile(s.T, (8, 1))
        shp_f = shp_e[k].reshape(T, 128, 9).transpose(1, 0, 2)
        shp_h[k] = shp_f.astype(BF16)
        shq_h[k] = (shp_f[:, :, jcols] * cgv[None, None, :]).astype(BF16)
        bas_h[k] = bas_e[k].reshape(NG, BP * 128, 8).transpose(0, 2, 1).astype(BF16)
        v = valid[k]
        e_slots = np.arange(T * 128)
        p_, t_ = e_slots % 128, e_slots // 128
        cols = lr[k] - (t_ // tpw) * 128
        ok = v & (cols >= 0) & (cols < 128)
        sm = np.zeros((128, T, 128), np.float32)
        sm[p_[ok], t_[ok], cols[ok]] = inv
        smat_h[k] = sm.astype(BF16)
    val = dict(T=T, NG=NG, tpw=tpw, nodes_of=nodes_of, sidx=sidx,
               shp_h=shp_h, bas_h=bas_h, smat_h=smat_h, shq_h=shq_h)
    _CACHE["key"], _CACHE["val"] = key, val
    return val


EXEC_NS = []


def _run_layer(nc, pre, ftab, oldT_by_core, lw):
    import os
    from concourse.bass_utils import run_bass_kernel_spmd
    in_maps = []
    for k in range(NCORES):
        m = dict(ftab=ftab,
                 sidx=pre["sidx"][k],
                 shp=pre["shp_h"][k],
                 shq=pre["shq_h"][k],
                 basisT=pre["bas_h"][k],
                 smat=pre["smat_h"][k],
                 oldT=oldT_by_core[k],
                 w1=lw["w1"], b1=lw["b1"], w2=lw["w2"], b2=lw["b2"],
                 w3=lw["w3"], lin0=lw["lin"][0], lin1=lw["lin"][1],
                 lin2=lw["lin"][2], gw0=lw["gw"][0], gw1=lw["gw"][1],
                 gb0=lw["gb"][0], gb1=lw["gb"][1])
        in_maps.append(m)
    trace = bool(os.environ.get("KERNEL_TRACE"))
    res = run_bass_kernel_spmd(nc, in_maps, list(range(NCORES)), trace=trace,
                               trace_cores=list(range(NCORES)) if trace else None)
    if trace and res.exec_time_ns is not None:
        EXEC_NS.append(res.exec_time_ns)
    return [res.results[k]["newT"] for k in range(NCORES)]


def _layer_weights(inputs, i, layer2):
    f32 = np.float32
    w3 = np.ascontiguousarray(inputs["mlp_w3"][i], f32)
    if layer2:
        w3p = w3.astype(BF16)
    else:
        w3p = np.empty((H, 576), BF16)
        for kg in range(9):
            p = L1_PATH_OF_K[kg]
            w3p[:, kg * C:(kg + 1) * C] = (
                w3[:, p * C:(p + 1) * C] * L1_CG_OF_K[kg]).astype(BF16)
    return dict(
        w1=np.ascontiguousarray(inputs["mlp_w1"][i], f32).astype(BF16),
        b1=np.ascontiguousarray(inputs["mlp_b1"][i], f32).reshape(H, 1),
        w2=np.ascontiguousarray(inputs["mlp_w2"][i], f32).astype(BF16),
        b2=np.ascontiguousarray(inputs["mlp_b2"][i], f32).reshape(H, 1),
        w3=w3p,
        lin=[np.ascontiguousarray(inputs["lin_self"][i, l], f32)
             for l in range(3)],
        gw=[np.ascontiguousarray(inputs["gate_w"][i, l], f32)
            for l in range(2)],
        gb=[np.ascontiguousarray(inputs["gate_b"][i, l], f32).reshape(C, 1)
            for l in range(2)],
    )


_KERNEL_CACHE = {}


def _get_kernels(T):
    if T not in _KERNEL_CACHE:
        _KERNEL_CACHE[T] = (build_layer_kernel(False, T),
                            build_layer_kernel(True, T))
    return _KERNEL_CACHE[T]


def _pack_ftab(table, ncols):
    out = np.zeros((N_NODES, ncols), BF16)
    used = min(ncols, table.shape[1])
    out[:, :used] = table[:, :used].astype(BF16)
    return out


def kernel(**inputs):
    positions = np.asarray(inputs["positions"], np.float32)
    species = np.asarray(inputs["species"]).astype(np.int64)
    senders = np.asarray(inputs["senders"]).astype(np.int64)
    receivers = np.asarray(inputs["receivers"]).astype(np.int64)

    pre = _prep(positions, senders, receivers)
    T = pre["T"]
    nc1, nc2 = _get_kernels(T)
    nodes_of = pre["nodes_of"]

    # initial features: x0 from species embedding (host; tiny)
    x0 = (np.asarray(inputs["embed"], np.float32)[species]
          @ np.asarray(inputs["w_proj"], np.float32))          # [N, 64]
    table = np.zeros((N_NODES, F), np.float32)
    table[:, 0:C] = x0

    # ---- layer 1 ----
    oldT = [_chunked_T(table[nodes_of[k]]) for k in range(NCORES)]
    lw = _layer_weights(inputs, 0, False)
    newT = _run_layer(nc1, pre, _pack_ftab(table, GCOLP_L1), oldT, lw)

    table2 = np.empty((N_NODES, F), np.float32)
    for k in range(NCORES):
        table2[nodes_of[k]] = _unchunk_T(newT[k])

    # ---- layer 2 ----
    lw = _layer_weights(inputs, 1, True)
    newT2 = _run_layer(nc2, pre, _pack_ftab(table2, GCOLP_L2), newT, lw)

    table3 = np.empty((N_NODES, F), np.float32)
    for k in range(NCORES):
        table3[nodes_of[k]] = _unchunk_T(newT2[k])

    # ---- output: reorder component-major -> reference layout + alpha ----
    t3 = table3.reshape(N_NODES, 9, C)
    out = np.empty((N_NODES, F), np.float32)
    out[:, 0:64] = t3[:, 0]
    out[:, 64:256] = (0.5 * t3[:, 1:4]).transpose(0, 2, 1).reshape(N_NODES, 192)
    out[:, 256:576] = (0.25 * t3[:, 4:9]).transpose(0, 2, 1).reshape(N_NODES, 320)
    return out



# revision 22
# speedup vs baseline: 1.3206x; 1.0027x over previous
"""NequIP GNN message-passing kernel for 8 Trainium2 NeuronCores — v2.

Receiver-sharded graph parallelism (per sharding hint): host LPT-assigns the
8192 nodes to 64 (core, window) bins of 128, each core owns 8 windows = 1024
nodes + their in-edges, sorted by window, padded to 128-edge tiles.

v2 device pipeline (vs v1): bf16 edge pipeline end-to-end with fp32 PSUM
accumulation; per-edge-scalar stages batched across 8-tile groups via
stride-0 broadcast access patterns; P-products collapsed per-path (w operand
broadcast over l1 components); CG-FMA stage merged into diagonal runs with
immediate scalars; segment-sum via paired-kg bf16 matmuls; radial MLP with
fused Silu activations; elementwise work split across Vector/GpSimd/Scalar.
"""
import math
import numpy as np
import ml_dtypes

BF16 = ml_dtypes.bfloat16

# ---------------- model constants ----------------
N_NODES, N_EDGES = 8192, 131072
C, H, NRAD = 64, 64, 8
R_MAX, AVG_NN = 5.0, 16.0
NCORES, NPC = 8, 1024
NW = NPC // 128
F = 9 * C
LS = (0, 1, 2)
PATHS = [(l1, l2, l3) for l1 in LS for l2 in LS for l3 in LS
         if abs(l1 - l2) <= l3 <= l1 + l2]
LOFF = {0: 0, 1: 1, 2: 4}
J_OF_L2 = {0: [0], 1: [1, 2, 3], 2: [4, 5, 6, 7, 8]}
BP = 8                      # tiles per group (batched in op free dims)
GCOLP_L2 = 640              # padded gather row (bf16): 1280B % 256 == 0
GCOLP_L1 = 128              # 256B % 256 == 0


# ---------------- real Clebsch-Gordan coefficients ----------------
def _cg_scalar(j1, m1, j2, m2, j3, m3):
    f = math.factorial
    if m1 + m2 != m3:
        return 0.0
    pre = ((2*j3+1) * f(j1+j2-j3) * f(j1-j2+j3) * f(-j1+j2+j3)
           / f(j1+j2+j3+1)) ** 0.5
    pre *= (f(j1+m1)*f(j1-m1)*f(j2+m2)*f(j2-m2)*f(j3+m3)*f(j3-m3)) ** 0.5
    s = 0.0
    for k in range(max(0, j2-j3-m1, j1+m2-j3), min(j1+j2-j3, j1-m1, j2+m2)+1):
        s += (-1)**k / (f(k)*f(j1+j2-j3-k)*f(j1-m1-k)
                        * f(j2+m2-k)*f(j3-j2+m1+k)*f(j3-j1-m2+k))
    return pre * s


def _U_real(l):
    U = np.zeros((2*l+1, 2*l+1), dtype=complex)
    s2 = 2 ** -0.5
    for m in range(-l, l+1):
        if m > 0:
            U[m+l, m+l] = (-1)**m * s2
            U[m+l, -m+l] = s2
        elif m == 0:
            U[l, l] = 1.0
        else:
            U[m+l, m+l] = 1j*s2
            U[m+l, -m+l] = -1j*(-1)**(-m)*s2
    return U


def _real_cg(l1, l2, l3):
    Cc = np.zeros((2*l1+1, 2*l2+1, 2*l3+1))
    for i1, m1 in enumerate(range(-l1, l1+1)):
        for i2, m2 in enumerate(range(-l2, l2+1)):
            m3 = m1 + m2
            if abs(m3) <= l3:
                Cc[i1, i2, m3+l3] = _cg_scalar(l1, m1, l2, m2, l3, m3)
    U1, U2, U3 = _U_real(l1), _U_real(l2), _U_real(l3)
    W = np.einsum('ia,jb,kc,abc->ijk', U1.conj(), U2.conj(), U3,
                  Cc.astype(complex))
    W = W.real if np.linalg.norm(W.real) >= np.linalg.norm(W.imag) else W.imag
    W = W / np.linalg.norm(W) * (2*l3+1) ** 0.5
    return np.asarray(W, dtype=np.float64)


CGS = [_real_cg(*p) for p in PATHS]


def build_schedule_l2():
    """Static TP structure for layer 2, grouped by l2.

    Per group: nblk, pops=[(path, ig0, ni, z0)], jlist, runs=[(j,z0,k0,L,cg)].
    """
    scheds = []
    for l2 in (0, 1, 2):
        ps = [p for p in range(len(PATHS)) if PATHS[p][1] == l2]
        blocks, block_of, pops = [], {}, []
        for p in ps:
            l1 = PATHS[p][0]
            ni = 2 * l1 + 1
            pops.append((p, LOFF[l1], ni, len(blocks)))
            for i in range(ni):
                block_of[(p, i)] = len(blocks)
                blocks.append((p, i))
        tset = set()
        for p in ps:
            l1, _, l3 = PATHS[p]
            cg = CGS[p]
            for i in range(2*l1+1):
                for j in range(2*l2+1):
                    for k in range(2*l3+1):
                        v = cg[i, j, k]
                        if abs(v) > 1e-12:
                            tset.add((LOFF[l2]+j, block_of[(p, i)],
                                      LOFF[l3]+k, round(float(v), 9)))
        runs, consumed = [], set()
        for t in sorted(tset):
            if t in consumed:
                continue
            j, z, k, cgv = t
            L = 0
            while (j, z+L, k+L, cgv) in tset and (j, z+L, k+L, cgv) not in consumed:
                consumed.add((j, z+L, k+L, cgv))
                L += 1
            runs.append((j, z, k, L, cgv))
        scheds.append(dict(l2=l2, nblk=len(blocks), pops=pops,
                           jlist=J_OF_L2[l2], runs=runs))
    return scheds


SCHED_L2 = build_schedule_l2()


def build_schedule2():
    """cg folded into per-(j,z) host scalars; FMA mostly tensor adds.

    Per group: nblk, pops, zjq=[(j, qoff)], runs_add=[(j,z0,k0,L)],
    runs_stt=[(j,z,k,ratio)]. qoff = column offset of (j,*) block in the
    concatenated shq table [128, T, 179]. cgfold[qoff+z] = cg of the primary
    (smallest-k) triple of (j,z); 0 for unused (z,j).
    """
    scheds, qoff, cgcols = [], 0, []
    for gi, sch in enumerate(SCHED_L2):
        nblk = sch["nblk"]
        tset = set()
        for (j, z, k, L, cg) in sch["runs"]:
            for i in range(L):
                tset.add((j, z + i, k + i, cg))
        per_jz = {}
        for (j, z, k, cg) in tset:
            per_jz.setdefault((j, z), []).append((k, cg))
        primary, runs_stt = set(), []
        cgf = {}
        for (j, z), ks in per_jz.items():
            ks.sort()
            k0, cg0 = ks[0]
            primary.add((j, z, k0))
            cgf[(j, z)] = cg0
            for (k1, cg1) in ks[1:]:
                runs_stt.append((j, z, k1, cg1 / cg0))
        runs_add, consumed = [], set()
        for t in sorted(primary):
            if t in consumed:
                continue
            j, z, k = t
            L = 0
            while (j, z + L, k + L) in primary and (j, z + L, k + L) not in consumed:
                consumed.add((j, z + L, k + L))
                L += 1
            runs_add.append((j, z, k, L))
        zjq = []
        for j in sch["jlist"]:
            # contiguous segments of z columns actually used by (j, z):
            # skips the ~28% of dense columns with cg == 0
            used = sorted(z for (jj, z) in cgf if jj == j)
            segs, s0, prev = [], None, None
            for z in used:
                if s0 is None:
                    s0 = prev = z
                elif z == prev + 1:
                    prev = z
                else:
                    segs.append((s0, prev - s0 + 1))
                    s0 = prev = z
            if s0 is not None:
                segs.append((s0, prev - s0 + 1))
            zjq.append((j, qoff, segs))
            for z in range(nblk):
                cgcols.append((j, cgf.get((j, z), 0.0)))
            qoff += nblk
        scheds.append(dict(nblk=nblk, pops=sch["pops"], zjq=zjq,
                           runs_add=runs_add, runs_stt=runs_stt))
    return scheds, cgcols


SCHED2_L2, SHQ_COLS = build_schedule2()
NSHQ = len(SHQ_COLS)

# layer-1 per-k path and cg (x is scalar-only: paths (0,l,l), j == k)
L1_PATH_OF_K = [0] + [1]*3 + [2]*5
L1_CG_OF_K = []
for _k in range(9):
    _p = L1_PATH_OF_K[_k]
    _l = PATHS[_p][2]
    _m = _k - LOFF[_l]
    L1_CG_OF_K.append(float(CGS[_p][0, _m, _m]))


# ---------------- host-side graph preprocessing ----------------
def edge_geometry(positions, senders, receivers):
    rel = (positions[receivers] - positions[senders]) / R_MAX
    d = np.linalg.norm(rel, axis=-1)
    u = rel / np.maximum(d, 1e-6)[:, None]
    x, y, z = u[:, 0], u[:, 1], u[:, 2]
    sh = np.empty((len(d), 9), np.float32)
    sh[:, 0] = 1.0
    sh[:, 1] = np.sqrt(3.0) * y
    sh[:, 2] = np.sqrt(3.0) * z
    sh[:, 3] = np.sqrt(3.0) * x
    sh[:, 4] = np.sqrt(15.0) * x * y
    sh[:, 5] = np.sqrt(15.0) * y * z
    sh[:, 6] = np.sqrt(5.0) / 2 * (3 * z * z - 1.0)
    sh[:, 7] = np.sqrt(15.0) * x * z
    sh[:, 8] = np.sqrt(15.0) / 2 * (x * x - y * y)
    freqs = np.arange(1, NRAD + 1, dtype=np.float64)
    xr = np.clip(d, 1e-4, 1.0)[:, None].astype(np.float64)
    basis = (np.sqrt(2.0) * np.sin(freqs * np.pi * xr) / xr).astype(np.float32)
    cut = (0.5 * (np.cos(np.pi * np.clip(d, 0.0, 1.0)) + 1.0)).astype(np.float32)
    return (sh * cut[:, None]).astype(np.float32), basis


def partition_graph(receivers):
    import heapq
    deg = np.bincount(receivers, minlength=N_NODES)
    order = np.argsort(-deg, kind="stable")
    nbins = NCORES * NW
    load = np.zeros(nbins, np.int64)
    cnt = np.zeros(nbins, np.int64)
    owner = np.empty(N_NODES, np.int32)
    local = np.empty(N_NODES, np.int32)
    heap = [(0, b) for b in range(nbins)]
    heapq.heapify(heap)
    for n in order:
        while True:
            l, b = heapq.heappop(heap)
            if cnt[b] < 128:
                break
        owner[n] = b // NW
        local[n] = (b % NW) * 128 + cnt[b]
        cnt[b] += 1
        load[b] += deg[n]
        if cnt[b] < 128:
            heapq.heappush(heap, (int(load[b]), b))
    nodes_of = np.empty((NCORES, NPC), np.int64)
    for n in range(N_NODES):
        nodes_of[owner[n], local[n]] = n
    return owner, local, nodes_of, int(load.max())


def build_core_edges(receivers, owner, local, tpw):
    T = NW * tpw
    perm = np.full((NCORES, T * 128), -1, np.int64)
    for k in range(NCORES):
        eids = np.where(owner[receivers] == k)[0]
        lr = local[receivers[eids]]
        o = np.argsort(lr, kind="stable")
        eids, lr = eids[o], lr[o]
        w_of = lr // 128
        for w in range(NW):
            sel = eids[w_of == w]
            assert len(sel) <= tpw * 128, "tiles-per-window overflow"
            base = w * tpw * 128
            perm[k, base:base + len(sel)] = sel
    return perm


# ---------------- bass kernel builder ----------------
def build_layer_kernel(layer2, T, debug=False):
    import concourse.bass as bass
    import concourse.bacc as bacc
    import concourse.tile as tile
    import concourse.mybir as mybir
    from contextlib import ExitStack

    fp32 = mybir.dt.float32
    bf16 = mybir.dt.bfloat16
    AF = mybir.ActivationFunctionType
    ALU = mybir.AluOpType

    NPATH = 15 if layer2 else 3
    GCOLP = GCOLP_L2 if layer2 else GCOLP_L1
    W3COL = NPATH * C if layer2 else 576   # L1 w3 host-expanded to 9 kg blocks
    E_PAD = T * 128
    NG = T // BP
    assert T % BP == 0 and T % NW == 0
    tpw = T // NW

    import os as _os
    STAGE = int(_os.environ.get("KV2_STAGE", "5"))
    nc = bacc.Bacc("TRN2", target_bir_lowering=False)

    ftab = nc.dram_tensor("ftab", [N_NODES, GCOLP], bf16, kind="ExternalInput")
    sidx = nc.dram_tensor("sidx", [128, E_PAD // 16], mybir.dt.int16,
                          kind="ExternalInput")
    shp_d = nc.dram_tensor("shp", [128, T, 9], bf16, kind="ExternalInput")
    shq_d = nc.dram_tensor("shq", [128, T, NSHQ], bf16, kind="ExternalInput")
    basT_d = nc.dram_tensor("basisT", [NG, 8, BP * 128], bf16,
                            kind="ExternalInput")
    smat_d = nc.dram_tensor("smat", [128, T, 128], bf16, kind="ExternalInput")
    oldT_d = nc.dram_tensor("oldT", [64, 9 * NPC], fp32, kind="ExternalInput")
    w1_d = nc.dram_tensor("w1", [8, H], bf16, kind="ExternalInput")
    b1_d = nc.dram_tensor("b1", [H, 1], fp32, kind="ExternalInput")
    w2_d = nc.dram_tensor("w2", [H, H], bf16, kind="ExternalInput")
    b2_d = nc.dram_tensor("b2", [H, 1], fp32, kind="ExternalInput")
    w3_d = nc.dram_tensor("w3", [H, W3COL], bf16, kind="ExternalInput")
    lin_d = [nc.dram_tensor(f"lin{l}", [C, C], fp32, kind="ExternalInput")
             for l in range(3)]
    gw_d = [nc.dram_tensor(f"gw{l}", [C, C], fp32, kind="ExternalInput")
            for l in range(2)]
    gb_d = [nc.dram_tensor(f"gb{l}", [C, 1], fp32, kind="ExternalInput")
            for l in range(2)]
    newT_d = nc.dram_tensor("newT", [64, 9 * NPC], fp32,
                            kind="ExternalOutput")
    if debug:
        dbg_xg = nc.dram_tensor("dbg_xg", [128, BP, GCOLP], fp32,
                                kind="ExternalOutput")
        dbg_w = nc.dram_tensor("dbg_w", [128, BP, W3COL], fp32,
                               kind="ExternalOutput")
        dbg_msgs = nc.dram_tensor("dbg_msgs", [128, BP, F], fp32,
                                  kind="ExternalOutput")
        dbg_agg = nc.dram_tensor("dbg_agg", [128, NW, 640], fp32,
                                 kind="ExternalOutput")

    with tile.TileContext(nc) as tc, ExitStack() as ctx:
        consts = ctx.enter_context(tc.tile_pool(name="consts", bufs=1))
        idx_sb = consts.tile([128, E_PAD // 16], mybir.dt.int16)
        nc.sync.dma_start(idx_sb[:], sidx[:])
        w1_sb = consts.tile([8, H], bf16)
        nc.sync.dma_start(w1_sb[:], w1_d[:])
        b1_sb = consts.tile([H, 1], fp32)
        nc.sync.dma_start(b1_sb[:], b1_d[:])
        w2_sb = consts.tile([H, H], bf16)
        nc.sync.dma_start(w2_sb[:], w2_d[:])
        b2_sb = consts.tile([H, 1], fp32)
        nc.sync.dma_start(b2_sb[:], b2_d[:])
        w3_sb = consts.tile([H, W3COL], bf16)
        nc.sync.dma_start(w3_sb[:], w3_d[:])
        lin_sb = [consts.tile([C, C], fp32, name=f"lin{l}", tag=f"lin{l}")
                  for l in range(3)]
        for l in range(3):
            nc.sync.dma_start(lin_sb[l][:], lin_d[l][:])
        gw_sb = [consts.tile([C, C], fp32, name=f"gw{l}", tag=f"gw{l}")
                 for l in range(2)]
        gb_sb = [consts.tile([C, 1], fp32, name=f"gb{l}", tag=f"gb{l}")
                 for l in range(2)]
        for l in range(2):
            nc.sync.dma_start(gw_sb[l][:], gw_d[l][:])
            nc.sync.dma_start(gb_sb[l][:], gb_d[l][:])
        agg_sb = consts.tile([64, NW, 2, 5, 128], fp32)

        with ExitStack() as psctx:
            iop = psctx.enter_context(tc.tile_pool(name="iop", bufs=2))
            aggt = psctx.enter_context(tc.tile_pool(name="aggt", bufs=1))
            wp = psctx.enter_context(tc.tile_pool(name="wp", bufs=1))
            msgp = psctx.enter_context(tc.tile_pool(name="msgp", bufs=2))
            shbp = psctx.enter_context(tc.tile_pool(name="shbp", bufs=1))
            pp = psctx.enter_context(tc.tile_pool(name="pp", bufs=1))
            zjp = psctx.enter_context(tc.tile_pool(name="zjp", bufs=1))
            zgp = psctx.enter_context(tc.tile_pool(name="zgp", bufs=1))
            h_ps = psctx.enter_context(
                tc.tile_pool(name="h_ps", bufs=1, space="PSUM"))
            w_ps_pool = psctx.enter_context(
                tc.tile_pool(name="w_ps", bufs=2, space="PSUM"))
            agg_pool = psctx.enter_context(
                tc.tile_pool(name="agg_ps", bufs=2, space="PSUM"))

            agg_open = {}

            for g in range(NG):
                t0 = g * BP
                xg = iop.tile([128, BP, GCOLP], bf16, tag="xg")
                nc.gpsimd.dma_gather(
                    out_ap=xg[:],
                    in_ap=ftab[:],
                    idxs_ap=idx_sb[:, g * (BP * 8):(g + 1) * (BP * 8)],
                    num_idxs=BP * 128,
                    num_idxs_reg=BP * 128,
                    elem_size=GCOLP,
                )
                shp_t = iop.tile([128, BP, 9], bf16, tag="shp")
                nc.sync.dma_start(shp_t[:], shp_d[:, t0:t0 + BP, :])
                if layer2:
                    shq_t = iop.tile([128, BP, NSHQ], bf16, tag="shq")
                    nc.sync.dma_start(shq_t[:], shq_d[:, t0:t0 + BP, :])
                smt = iop.tile([128, BP, 128], bf16, tag="smt")
                nc.sync.dma_start(smt[:], smat_d[:, t0:t0 + BP, :])
                bas = iop.tile([8, BP * 128], bf16, tag="bas")
                nc.sync.dma_start(bas[:], basT_d[g, :, :])

                if STAGE < 2:
                    continue
                if not layer2:
                    # sh broadcast table for L1 (ScalarE, stride-0 copy)
                    shB = shbp.tile([128, BP, 9, C], bf16, tag="shB")
                    nc.scalar.copy(
                        shB[:],
                        shp_t[:, :, :, None].broadcast_to((128, BP, 9, C)))

                # ---- radial MLP (transposed; fused Silu) ----
                h1s = iop.tile([H, BP * 128], bf16, tag="h1s")
                h2s = iop.tile([H, BP * 128], bf16, tag="h2s")
                sg = iop.tile([H, BP * 128], bf16, tag="sg")
                for c0 in range(0, BP * 128, 512):
                    h1p = h_ps.tile([H, 512], fp32, tag="h")
                    nc.tensor.matmul(h1p[:], w1_sb[:],
                                     bas[:, c0:c0 + 512], start=True, stop=True)
                    nc.scalar.activation(sg[:, c0:c0 + 512], h1p[:],
                                         AF.Sigmoid, bias=b1_sb[:, 0:1])
                    nc.vector.scalar_tensor_tensor(
                        out=h1s[:, c0:c0 + 512], in0=h1p[:],
                        scalar=b1_sb[:, 0:1], in1=sg[:, c0:c0 + 512],
                        op0=ALU.add, op1=ALU.mult)
                for c0 in range(0, BP * 128, 512):
                    h2p = h_ps.tile([H, 512], fp32, tag="h")
                    nc.tensor.matmul(h2p[:], w2_sb[:],
                                     h1s[:, c0:c0 + 512], start=True, stop=True)
                    nc.scalar.activation(sg[:, c0:c0 + 512], h2p[:],
                                         AF.Sigmoid, bias=b2_sb[:, 0:1])
                    nc.vector.scalar_tensor_tensor(
                        out=h2s[:, c0:c0 + 512], in0=h2p[:],
                        scalar=b2_sb[:, 0:1], in1=sg[:, c0:c0 + 512],
                        op0=ALU.add, op1=ALU.mult)

                # ---- per-tile edge weights w = h2s_t.T @ W3 (bf16 out) ----
                # one 1-bank PSUM tile per 512-col chunk, double-buffered:
                # tile t+1's matmul overlaps tile t's PSUM->SBUF copy
                w_sb = wp.tile([128, BP, W3COL], bf16, tag="wsb")
                for bt in range(BP):
                    for c0 in range(0, W3COL, 512):
                        c1 = min(c0 + 512, W3COL)
                        w_ps = w_ps_pool.tile([128, 512], fp32, tag="wps")
                        nc.tensor.matmul(w_ps[:, 0:c1 - c0],
                                         h2s[:, bt * 128:(bt + 1) * 128],
                                         w3_sb[:, c0:c1], start=True, stop=True)
                        nc.scalar.copy(w_sb[:, bt, c0:c1], w_ps[:, 0:c1 - c0])

                if STAGE < 3:
                    continue
                # ---- tensor product ----
                msgs = msgp.tile([128, BP, F], bf16, tag="msgs")
                if layer2:
                    _emit_tp_l2(nc, ALU, xg, w_sb, shq_t, msgs, pp, zjp, zgp)
                else:
                    _emit_tp_l1(nc, ALU, xg, w_sb, shB, msgs, pp)

                if debug and g == 0:
                    dxg = pp.tile([128, BP, GCOLP], fp32, tag="dxg")
                    nc.vector.tensor_copy(out=dxg[:], in_=xg[:])
                    nc.sync.dma_start(dbg_xg[:], dxg[:])
                    dw = pp.tile([128, BP, W3COL], fp32, tag="dw")
                    nc.vector.tensor_copy(out=dw[:], in_=w_sb[:])
                    nc.sync.dma_start(dbg_w[:], dw[:])
                    dmg = pp.tile([128, BP, F], fp32, tag="dmg")
                    nc.vector.tensor_copy(out=dmg[:], in_=msgs[:])
                    nc.sync.dma_start(dbg_msgs[:], dmg[:])

                if STAGE < 4:
                    continue
                # ---- segment sum: paired-kg bf16 matmuls, PSUM-accumulated ----
                for bt in range(BP):
                    t = t0 + bt
                    w, t_in_w = t // tpw, t % tpw
                    if w not in agg_open:
                        agg_open[w] = agg_pool.tile([128, 640], fp32,
                                                    name="aggps", tag="aggps")
                    ps = agg_open[w]
                    first, last = t_in_w == 0, t_in_w == tpw - 1
                    for pair in range(5):
                        c0 = pair * 128
                        cw = 128 if pair < 4 else 64
                        nc.tensor.matmul(
                            ps[0:cw, c0:c0 + 128],
                            msgs[:, bt, c0:c0 + cw],
                            smt[:, bt, :],
                            start=first and pair in (0, 4),
                            stop=last, skip_group_check=True)
                    if last:
                        aps = agg_open.pop(w)
                        afp = aggt.tile([128, 640], fp32, tag="afp")
                        nc.scalar.copy(afp[:, 0:512], aps[:, 0:512])
                        nc.scalar.copy(afp[0:64, 512:640], aps[0:64, 512:640])
                        nc.sync.dma_start(
                            agg_sb[:, w, 0, :, :],
                            afp[0:64, :].rearrange("p (q n) -> p q n", q=5))
                        nc.sync.dma_start(
                            agg_sb[:, w, 1, 0:4, :],
                            afp[64:128, 0:512].rearrange("p (q n) -> p q n",
                                                         q=4))



        # ---------------- per-window node update ----------------
        if STAGE < 5:
            with ExitStack() as upctx:
                upt = upctx.enter_context(tc.tile_pool(name="upt", bufs=2))
                for w in range(NW):
                    tmp = upt.tile([64, 9, 128], fp32, tag="pass")
                    nc.sync.dma_start(
                        tmp[:], oldT_d[:, :].rearrange(
                            "p (q n) -> p q n",
                            q=9)[:, :, w * 128:(w + 1) * 128])
                    nc.sync.dma_start(
                        newT_d[:, :].rearrange(
                            "p (q n) -> p q n",
                            q=9)[:, :, w * 128:(w + 1) * 128],
                        tmp[:])
            nc.compile()
            return nc
        with ExitStack() as upctx:
            y_pool = upctx.enter_context(
                tc.tile_pool(name="y_ps", bufs=2, space="PSUM"))
            g_pool = upctx.enter_context(
                tc.tile_pool(name="g_ps", bufs=2, space="PSUM"))
            upt = upctx.enter_context(tc.tile_pool(name="upt", bufs=2))
            for w in range(NW):
                oldw = upt.tile([64, 9, 128], fp32, tag="oldw")
                nc.sync.dma_start(
                    oldw[:], oldT_d[:, :].rearrange(
                        "p (q n) -> p q n", q=9)[:, :, w * 128:(w + 1) * 128])
                neww = upt.tile([64, 9, 128], fp32, tag="neww")
                y_ps = y_pool.tile([64, 9 * 128], fp32, tag="yps")
                for kg in range(9):
                    l = 0 if kg == 0 else (1 if kg <= 3 else 2)
                    nc.tensor.matmul(
                        y_ps[:, kg * 128:(kg + 1) * 128],
                        lin_sb[l][:],
                        agg_sb[:, w, kg % 2, kg // 2, :],
                        start=kg in (0, 4, 8), stop=True,
                        skip_group_check=True)
                y0g = upt.tile([C, 128], fp32, tag="y0g")
                nc.scalar.activation(y0g[:], y_ps[:, 0:128], AF.Sigmoid)
                y0s = upt.tile([C, 128], fp32, tag="y0s")
                nc.vector.tensor_tensor(out=y0s[:], in0=y_ps[:, 0:128],
                                        in1=y0g[:], op=ALU.mult)
                nc.vector.tensor_tensor(out=neww[:, 0, :], in0=y0s[:],
                                        in1=oldw[:, 0, :], op=ALU.add)
                g_ps = g_pool.tile([C, 2, 128], fp32, tag="gps")
                for l in (1, 2):
                    nc.tensor.matmul(g_ps[:, l - 1, :], gw_sb[l - 1][:],
                                     neww[:, 0, :], start=(l == 1), stop=True,
                                     skip_group_check=True)
                gts = upt.tile([C, 2, 128], fp32, tag="gts")
                for l in (1, 2):
                    nc.scalar.activation(gts[:, l - 1, :], g_ps[:, l - 1, :],
                                         AF.Sigmoid, bias=gb_sb[l - 1][:, 0:1])
                gy = upt.tile([C, 8, 128], fp32, tag="gy")
                nc.vector.tensor_tensor(
                    out=gy[:, 0:3, :],
                    in0=y_ps[:].rearrange("p (q n) -> p q n", q=9)[:, 1:4, :],
                    in1=gts[:, 0:1, :].broadcast_to((C, 3, 128)),
                    op=ALU.mult)
                nc.vector.tensor_tensor(
                    out=gy[:, 3:8, :],
                    in0=y_ps[:].rearrange("p (q n) -> p q n", q=9)[:, 4:9, :],
                    in1=gts[:, 1:2, :].broadcast_to((C, 5, 128)),
                    op=ALU.mult)
                nc.vector.tensor_tensor(out=neww[:, 1:9, :], in0=gy[:],
                                        in1=oldw[:, 1:9, :], op=ALU.add)
                nc.sync.dma_start(
                    newT_d[:, :].rearrange("p (q n) -> p q n",
                                           q=9)[:, :, w * 128:(w + 1) * 128],
                    neww[:])

    nc.compile()
    return nc


def _emit_tp_l2(nc, ALU, xg, w_sb, shq_t, msgs, pp, zjp, zgp):
    """P products, cg-folded zjQ scalings, FMA as adds (+8 ratio stts).

    All on DVE: single in-order queue, no cross-engine ping-pong; GpSimd
    has ~9us fixed cost per elementwise op and ScalarE cannot multiply
    two tensors, so the TP bulk lives here.
    """
    import concourse.mybir as mybir
    bf16 = mybir.dt.bfloat16

    GP_JS = ()          # GpSimd streams ~0.4 elem/cyc and zjG bufs=1
                        # serialized groups -> ~19us DVE stall per group;
                        # cheaper to keep all zjq on DVE (nnz-only now)
    EXPC = 11           # ping-pong expansion chunk: 2 x 8x11x64 bf16 = 22.5KB
    P0 = pp.tile([128, BP, SCHED2_L2[0]["nblk"] * C], bf16, name="P0", tag="P0")
    P12 = pp.tile([128, BP, SCHED2_L2[2]["nblk"] * C], bf16,
                  name="P12", tag="P12")
    zj12 = zjp.tile([128, BP, SCHED2_L2[2]["nblk"] * C], bf16,
                    name="zj12", tag="zj12")
    exps = [zgp.tile([128, BP, EXPC, C], bf16, name=f"exp{i}", tag=f"exp{i}")
            for i in range(2)]
    ppg = [0]

    def emit_zjq(eng, dst, P, qoff, segs):
        # ScalarE pre-broadcasts shq cols to C width (stride-0 src is fine
        # on ACT); DVE then multiplies with unit-stride in1 -> 2x_1P mode
        # (the old stride-0 broadcast in1 forced 1x). Ping-pong chunks of
        # <= EXPC cols so ScalarE expansion overlaps DVE consumption.
        chunks, cur, used = [], [], 0
        for (z0, L) in segs:
            while L > 0:
                if used == EXPC:
                    chunks.append(cur)
                    cur, used = [], 0
                take = min(L, EXPC - used)
                cur.append((z0, take, used))
                z0, L, used = z0 + take, L - take, used + take
        if cur:
            chunks.append(cur)
        for ch in chunks:
            exp = exps[ppg[0]]
            ppg[0] ^= 1
            for (z0, L, eo) in ch:
                nc.scalar.copy(
                    exp[:, :, eo:eo + L, :],
                    shq_t[:, :, qoff + z0:qoff + z0 + L, None].broadcast_to(
                        (128, BP, L, C)))
            for (z0, L, eo) in ch:
                eng.tensor_tensor(
                    out=dst[:].rearrange("p b (n c) -> p b n c",
                                         c=C)[:, :, z0:z0 + L, :],
                    in0=P[:].rearrange("p b (n c) -> p b n c",
                                       c=C)[:, :, z0:z0 + L, :],
                    in1=exp[:, :, eo:eo + L, :],
                    op=ALU.mult)

    def emit_fma(sch, j, zj):
        for (jj, z0, k0, L) in sch["runs_add"]:
            if jj != j:
                continue
            nc.vector.tensor_tensor(
                out=msgs[:, :, k0 * C:(k0 + L) * C],
                in0=zj[:, :, z0 * C:(z0 + L) * C],
                in1=msgs[:, :, k0 * C:(k0 + L) * C],
                op=ALU.add)
        for (jj, z, k, ratio) in sch["runs_stt"]:
            if jj != j:
                continue
            nc.vector.scalar_tensor_tensor(
                out=msgs[:, :, k * C:(k + 1) * C],
                in0=zj[:, :, z * C:(z + 1) * C],
                scalar=float(ratio),
                in1=msgs[:, :, k * C:(k + 1) * C],
                op0=ALU.mult, op1=ALU.add)

    for gi, sch in enumerate(SCHED2_L2):
        nblk = sch["nblk"]
        P = P0 if gi == 0 else P12
        for (p, ig0, ni, z0) in sch["pops"]:
            nc.vector.tensor_tensor(
                out=P[:].rearrange("p b (n c) -> p b n c",
                                   c=C)[:, :, z0:z0 + ni, :],
                in0=xg[:].rearrange("p b (n c) -> p b n c",
                                    c=C)[:, :, ig0:ig0 + ni, :],
                in1=w_sb[:, :, None, p * C:(p + 1) * C].broadcast_to(
                    (128, BP, ni, C)),
                op=ALU.mult)
        if gi > 0:
            # kick GpSimd's share first so it overlaps DVE's other j's
            for (j, qoff, segs) in sch["zjq"]:
                if j in GP_JS:
                    emit_zjq(nc.gpsimd, zjG[j], P, qoff, segs)
        for (j, qoff, segs) in sch["zjq"]:
            if gi > 0 and j in GP_JS:
                continue
            dst = msgs if gi == 0 else zj12
            emit_zjq(nc.vector, dst, P, qoff, segs)
            if gi == 0:
                continue
            emit_fma(sch, j, zj12)
        if gi > 0:
            for j in GP_JS:
                if any(jj == j for (jj, _, _) in sch["zjq"]):
                    emit_fma(sch, j, zjG[j])


def _emit_tp_l1(nc, ALU, xg, w_sb, shB, msgs, pp):
    """msgs[k] = x * w'_k * sh_k ; w' host-expanded+cg-folded to 9 kg blocks."""
    import concourse.mybir as mybir
    bf16 = mybir.dt.bfloat16
    tmp = pp.tile([128, BP, 9, C], bf16, name="l1tmp", tag="l1tmp")
    nc.vector.tensor_tensor(
        out=tmp[:],
        in0=w_sb[:].rearrange("p b (n c) -> p b n c", c=C),
        in1=shB[:],
        op=ALU.mult)
    nc.vector.tensor_tensor(
        out=msgs[:].rearrange("p b (n c) -> p b n c", c=C),
        in0=tmp[:],
        in1=xg[:, :, None, 0:C].broadcast_to((128, BP, 9, C)),
        op=ALU.mult)


# ---------------- host orchestration ----------------
def _chunked_T(feats_own):
    """[NPC, 576] -> kg-blocked transposed [64, 9*NPC]."""
    out = np.empty((64, 9 * NPC), np.float32)
    for kg in range(9):
        out[:, kg * NPC:(kg + 1) * NPC] = feats_own[:, kg * 64:(kg + 1) * 64].T
    return out


def _unchunk_T(newT):
    """[64, 9*NPC] -> [NPC, 576]."""
    out = np.empty((NPC, 576), np.float32)
    for kg in range(9):
        out[:, kg * 64:(kg + 1) * 64] = newT[:, kg * NPC:(kg + 1) * NPC].T
    return out


_CACHE = {}


def _prep(positions, senders, receivers):
    key = (senders.tobytes(), receivers.tobytes(), positions.tobytes())
    if _CACHE.get("key") == key:
        return _CACHE["val"]
    sh_eff, basis = edge_geometry(positions, senders, receivers)
    owner, local, nodes_of, _ = partition_graph(receivers)
    deg_bin = np.zeros(NCORES * NW, np.int64)
    np.add.at(deg_bin, owner[receivers] * NW + local[receivers] // 128, 1)
    tpw = (int(deg_bin.max()) + 127) // 128
    T = NW * tpw
    assert T % BP == 0
    perm = build_core_edges(receivers, owner, local, tpw)

    valid = perm >= 0
    eg = np.where(valid, perm, 0)
    snd = np.where(valid, senders[eg], 0).astype(np.int16)      # [NC, T*128]
    shp_e = sh_eff[eg] * valid[..., None]                        # [NC, T*128, 9]
    bas_e = basis[eg] * valid[..., None]                         # [NC, T*128, 8]
    lr = np.where(valid, local[receivers[eg]], 0)

    NG = T // BP
    inv = np.float32(1.0 / np.sqrt(AVG_NN))
    sidx = np.empty((NCORES, 128, T * 128 // 16), np.int16)
    shp_h = np.empty((NCORES, 128, T, 9), BF16)
    shq_h = np.empty((NCORES, 128, T, NSHQ), BF16)
    jcols = np.array([j for (j, cg) in SHQ_COLS])
    cgv = np.array([cg for (j, cg) in SHQ_COLS], np.float32)
    bas_h = np.empty((NCORES, NG, 8, BP * 128), BF16)
    smat_h = np.zeros((NCORES, 128, T, 128), BF16)
    for k in range(NCORES):
        s = snd[k].reshape(T * 8, 16)
        sidx[k] = np.tile(s.T, (8, 1))
        shp_f = shp_e[k].reshape(T, 128, 9).transpose(1, 0, 2)
        shp_h[k] = shp_f.astype(BF16)
        shq_h[k] = (shp_f[:, :, jcols] * cgv[None, None, :]).astype(BF16)
        bas_h[k] = bas_e[k].reshape(NG, BP * 128, 8).transpose(0, 2, 1).astype(BF16)
        v = valid[k]
        e_slots = np.arange(T * 128)
        p_, t_ = e_slots % 128, e_slots // 128
        cols = lr[k] - (t_ // tpw) * 128
        ok = v & (cols >= 0) & (cols < 128)
        sm = np.zeros((128, T, 128), np.float32)
        sm[p_[ok], t_[ok], cols[ok]] = inv
        smat_h[k] = sm.astype(BF16)
    val = dict(T=T, NG=NG, tpw=tpw, nodes_of=nodes_of, sidx=sidx,
               shp_h=shp_h, bas_h=bas_h, smat_h=smat_h, shq_h=shq_h)
    _CACHE["key"], _CACHE["val"] = key, val
    return val


EXEC_NS = []


def _run_layer(nc, pre, ftab, oldT_by_core, lw):
    import os
    from concourse.bass_utils import run_bass_kernel_spmd
    in_maps = []
    for k in range(NCORES):
        m = dict(ftab=ftab,
                 sidx=pre["sidx"][k],
                 shp=pre["shp_h"][k],
                 shq=pre["shq_h"][k],
                 basisT=pre["bas_h"][k],
                 smat=pre["smat_h"][k],
                 oldT=oldT_by_core[k],
                 w1=lw["w1"], b1=lw["b1"], w2=lw["w2"], b2=lw["b2"],
                 w3=lw["w3"], lin0=lw["lin"][0], lin1=lw["lin"][1],
                 lin2=lw["lin"][2], gw0=lw["gw"][0], gw1=lw["gw"][1],
                 gb0=lw["gb"][0], gb1=lw["gb"][1])
        in_maps.append(m)
    trace = bool(os.environ.get("KERNEL_TRACE"))
    res = run_bass_kernel_spmd(nc, in_maps, list(range(NCORES)), trace=trace,
                               trace_cores=list(range(NCORES)) if trace else None)
    if trace and res.exec_time_ns is not None:
        EXEC_NS.append(res.exec_time_ns)
    return [res.results[k]["newT"] for k in range(NCORES)]


def _layer_weights(inputs, i, layer2):
    f32 = np.float32
    w3 = np.ascontiguousarray(inputs["mlp_w3"][i], f32)
    if layer2:
        w3p = w3.astype(BF16)
    else:
        w3p = np.empty((H, 576), BF16)
        for kg in range(9):
            p = L1_PATH_OF_K[kg]
            w3p[:, kg * C:(kg + 1) * C] = (
                w3[:, p * C:(p + 1) * C] * L1_CG_OF_K[kg]).astype(BF16)
    return dict(
        w1=np.ascontiguousarray(inputs["mlp_w1"][i], f32).astype(BF16),
        b1=np.ascontiguousarray(inputs["mlp_b1"][i], f32).reshape(H, 1),
        w2=np.ascontiguousarray(inputs["mlp_w2"][i], f32).astype(BF16),
        b2=np.ascontiguousarray(inputs["mlp_b2"][i], f32).reshape(H, 1),
        w3=w3p,
        lin=[np.ascontiguousarray(inputs["lin_self"][i, l], f32)
             for l in range(3)],
        gw=[np.ascontiguousarray(inputs["gate_w"][i, l], f32)
            for l in range(2)],
        gb=[np.ascontiguousarray(inputs["gate_b"][i, l], f32).reshape(C, 1)
            for l in range(2)],
    )


_KERNEL_CACHE = {}


def _get_kernels(T):
    if T not in _KERNEL_CACHE:
        _KERNEL_CACHE[T] = (build_layer_kernel(False, T),
                            build_layer_kernel(True, T))
    return _KERNEL_CACHE[T]


def _pack_ftab(table, ncols):
    out = np.zeros((N_NODES, ncols), BF16)
    used = min(ncols, table.shape[1])
    out[:, :used] = table[:, :used].astype(BF16)
    return out


def kernel(**inputs):
    positions = np.asarray(inputs["positions"], np.float32)
    species = np.asarray(inputs["species"]).astype(np.int64)
    senders = np.asarray(inputs["senders"]).astype(np.int64)
    receivers = np.asarray(inputs["receivers"]).astype(np.int64)

    pre = _prep(positions, senders, receivers)
    T = pre["T"]
    nc1, nc2 = _get_kernels(T)
    nodes_of = pre["nodes_of"]

    # initial features: x0 from species embedding (host; tiny)
    x0 = (np.asarray(inputs["embed"], np.float32)[species]
          @ np.asarray(inputs["w_proj"], np.float32))          # [N, 64]
    table = np.zeros((N_NODES, F), np.float32)
    table[:, 0:C] = x0

    # ---- layer 1 ----
    oldT = [_chunked_T(table[nodes_of[k]]) for k in range(NCORES)]
    lw = _layer_weights(inputs, 0, False)
    newT = _run_layer(nc1, pre, _pack_ftab(table, GCOLP_L1), oldT, lw)

    table2 = np.empty((N_NODES, F), np.float32)
    for k in range(NCORES):
        table2[nodes_of[k]] = _unchunk_T(newT[k])

    # ---- layer 2 ----
    lw = _layer_weights(inputs, 1, True)
    newT2 = _run_layer(nc2, pre, _pack_ftab(table2, GCOLP_L2), newT, lw)

    table3 = np.empty((N_NODES, F), np.float32)
    for k in range(NCORES):
        table3[nodes_of[k]] = _unchunk_T(newT2[k])

    # ---- output: reorder component-major -> reference layout + alpha ----
    t3 = table3.reshape(N_NODES, 9, C)
    out = np.empty((N_NODES, F), np.float32)
    out[:, 0:64] = t3[:, 0]
    out[:, 64:256] = (0.5 * t3[:, 1:4]).transpose(0, 2, 1).reshape(N_NODES, 192)
    out[:, 256:576] = (0.25 * t3[:, 4:9]).transpose(0, 2, 1).reshape(N_NODES, 320)
    return out



# revision 23
# speedup vs baseline: 1.3419x; 1.0162x over previous
"""NequIP GNN message-passing kernel for 8 Trainium2 NeuronCores — v2.

Receiver-sharded graph parallelism (per sharding hint): host LPT-assigns the
8192 nodes to 64 (core, window) bins of 128, each core owns 8 windows = 1024
nodes + their in-edges, sorted by window, padded to 128-edge tiles.

v2 device pipeline (vs v1): bf16 edge pipeline end-to-end with fp32 PSUM
accumulation; per-edge-scalar stages batched across 8-tile groups via
stride-0 broadcast access patterns; P-products collapsed per-path (w operand
broadcast over l1 components); CG-FMA stage merged into diagonal runs with
immediate scalars; segment-sum via paired-kg bf16 matmuls; radial MLP with
fused Silu activations; elementwise work split across Vector/GpSimd/Scalar.
"""
import math
import numpy as np
import ml_dtypes

BF16 = ml_dtypes.bfloat16

# ---------------- model constants ----------------
N_NODES, N_EDGES = 8192, 131072
C, H, NRAD = 64, 64, 8
R_MAX, AVG_NN = 5.0, 16.0
NCORES, NPC = 8, 1024
NW = NPC // 128
F = 9 * C
LS = (0, 1, 2)
PATHS = [(l1, l2, l3) for l1 in LS for l2 in LS for l3 in LS
         if abs(l1 - l2) <= l3 <= l1 + l2]
LOFF = {0: 0, 1: 1, 2: 4}
J_OF_L2 = {0: [0], 1: [1, 2, 3], 2: [4, 5, 6, 7, 8]}
BP = 8                      # tiles per group (batched in op free dims)
GCOLP_L2 = 640              # padded gather row (bf16): 1280B % 256 == 0
GCOLP_L1 = 128              # 256B % 256 == 0


# ---------------- real Clebsch-Gordan coefficients ----------------
def _cg_scalar(j1, m1, j2, m2, j3, m3):
    f = math.factorial
    if m1 + m2 != m3:
        return 0.0
    pre = ((2*j3+1) * f(j1+j2-j3) * f(j1-j2+j3) * f(-j1+j2+j3)
           / f(j1+j2+j3+1)) ** 0.5
    pre *= (f(j1+m1)*f(j1-m1)*f(j2+m2)*f(j2-m2)*f(j3+m3)*f(j3-m3)) ** 0.5
    s = 0.0
    for k in range(max(0, j2-j3-m1, j1+m2-j3), min(j1+j2-j3, j1-m1, j2+m2)+1):
        s += (-1)**k / (f(k)*f(j1+j2-j3-k)*f(j1-m1-k)
                        * f(j2+m2-k)*f(j3-j2+m1+k)*f(j3-j1-m2+k))
    return pre * s


def _U_real(l):
    U = np.zeros((2*l+1, 2*l+1), dtype=complex)
    s2 = 2 ** -0.5
    for m in range(-l, l+1):
        if m > 0:
            U[m+l, m+l] = (-1)**m * s2
            U[m+l, -m+l] = s2
        elif m == 0:
            U[l, l] = 1.0
        else:
            U[m+l, m+l] = 1j*s2
            U[m+l, -m+l] = -1j*(-1)**(-m)*s2
    return U


def _real_cg(l1, l2, l3):
    Cc = np.zeros((2*l1+1, 2*l2+1, 2*l3+1))
    for i1, m1 in enumerate(range(-l1, l1+1)):
        for i2, m2 in enumerate(range(-l2, l2+1)):
            m3 = m1 + m2
            if abs(m3) <= l3:
                Cc[i1, i2, m3+l3] = _cg_scalar(l1, m1, l2, m2, l3, m3)
    U1, U2, U3 = _U_real(l1), _U_real(l2), _U_real(l3)
    W = np.einsum('ia,jb,kc,abc->ijk', U1.conj(), U2.conj(), U3,
                  Cc.astype(complex))
    W = W.real if np.linalg.norm(W.real) >= np.linalg.norm(W.imag) else W.imag
    W = W / np.linalg.norm(W) * (2*l3+1) ** 0.5
    return np.asarray(W, dtype=np.float64)


CGS = [_real_cg(*p) for p in PATHS]


def build_schedule_l2():
    """Static TP structure for layer 2, grouped by l2.

    Per group: nblk, pops=[(path, ig0, ni, z0)], jlist, runs=[(j,z0,k0,L,cg)].
    """
    scheds = []
    for l2 in (0, 1, 2):
        ps = [p for p in range(len(PATHS)) if PATHS[p][1] == l2]
        blocks, block_of, pops = [], {}, []
        for p in ps:
            l1 = PATHS[p][0]
            ni = 2 * l1 + 1
            pops.append((p, LOFF[l1], ni, len(blocks)))
            for i in range(ni):
                block_of[(p, i)] = len(blocks)
                blocks.append((p, i))
        tset = set()
        for p in ps:
            l1, _, l3 = PATHS[p]
            cg = CGS[p]
            for i in range(2*l1+1):
                for j in range(2*l2+1):
                    for k in range(2*l3+1):
                        v = cg[i, j, k]
                        if abs(v) > 1e-12:
                            tset.add((LOFF[l2]+j, block_of[(p, i)],
                                      LOFF[l3]+k, round(float(v), 9)))
        runs, consumed = [], set()
        for t in sorted(tset):
            if t in consumed:
                continue
            j, z, k, cgv = t
            L = 0
            while (j, z+L, k+L, cgv) in tset and (j, z+L, k+L, cgv) not in consumed:
                consumed.add((j, z+L, k+L, cgv))
                L += 1
            runs.append((j, z, k, L, cgv))
        scheds.append(dict(l2=l2, nblk=len(blocks), pops=pops,
                           jlist=J_OF_L2[l2], runs=runs))
    return scheds


SCHED_L2 = build_schedule_l2()


def build_schedule2():
    """cg folded into per-(j,z) host scalars; FMA mostly tensor adds.

    Per group: nblk, pops, zjq=[(j, qoff)], runs_add=[(j,z0,k0,L)],
    runs_stt=[(j,z,k,ratio)]. qoff = column offset of (j,*) block in the
    concatenated shq table [128, T, 179]. cgfold[qoff+z] = cg of the primary
    (smallest-k) triple of (j,z); 0 for unused (z,j).
    """
    scheds, qoff, cgcols = [], 0, []
    for gi, sch in enumerate(SCHED_L2):
        nblk = sch["nblk"]
        tset = set()
        for (j, z, k, L, cg) in sch["runs"]:
            for i in range(L):
                tset.add((j, z + i, k + i, cg))
        per_jz = {}
        for (j, z, k, cg) in tset:
            per_jz.setdefault((j, z), []).append((k, cg))
        primary, runs_stt = set(), []
        cgf = {}
        for (j, z), ks in per_jz.items():
            ks.sort()
            k0, cg0 = ks[0]
            primary.add((j, z, k0))
            cgf[(j, z)] = cg0
            for (k1, cg1) in ks[1:]:
                runs_stt.append((j, z, k1, cg1 / cg0))
        runs_add, consumed = [], set()
        for t in sorted(primary):
            if t in consumed:
                continue
            j, z, k = t
            L = 0
            while (j, z + L, k + L) in primary and (j, z + L, k + L) not in consumed:
                consumed.add((j, z + L, k + L))
                L += 1
            runs_add.append((j, z, k, L))
        zjq = []
        for j in sch["jlist"]:
            # contiguous segments of z columns actually used by (j, z):
            # skips the ~28% of dense columns with cg == 0
            used = sorted(z for (jj, z) in cgf if jj == j)
            segs, s0, prev = [], None, None
            for z in used:
                if s0 is None:
                    s0 = prev = z
                elif z == prev + 1:
                    prev = z
                else:
                    segs.append((s0, prev - s0 + 1))
                    s0 = prev = z
            if s0 is not None:
                segs.append((s0, prev - s0 + 1))
            zjq.append((j, qoff, segs))
            for z in range(nblk):
                cgcols.append((j, cgf.get((j, z), 0.0)))
            qoff += nblk
        scheds.append(dict(nblk=nblk, pops=sch["pops"], zjq=zjq,
                           runs_add=runs_add, runs_stt=runs_stt))
    return scheds, cgcols


SCHED2_L2, SHQ_COLS = build_schedule2()
NSHQ = len(SHQ_COLS)

# layer-1 per-k path and cg (x is scalar-only: paths (0,l,l), j == k)
L1_PATH_OF_K = [0] + [1]*3 + [2]*5
L1_CG_OF_K = []
for _k in range(9):
    _p = L1_PATH_OF_K[_k]
    _l = PATHS[_p][2]
    _m = _k - LOFF[_l]
    L1_CG_OF_K.append(float(CGS[_p][0, _m, _m]))


# ---------------- host-side graph preprocessing ----------------
def edge_geometry(positions, senders, receivers):
    rel = (positions[receivers] - positions[senders]) / R_MAX
    d = np.linalg.norm(rel, axis=-1)
    u = rel / np.maximum(d, 1e-6)[:, None]
    x, y, z = u[:, 0], u[:, 1], u[:, 2]
    sh = np.empty((len(d), 9), np.float32)
    sh[:, 0] = 1.0
    sh[:, 1] = np.sqrt(3.0) * y
    sh[:, 2] = np.sqrt(3.0) * z
    sh[:, 3] = np.sqrt(3.0) * x
    sh[:, 4] = np.sqrt(15.0) * x * y
    sh[:, 5] = np.sqrt(15.0) * y * z
    sh[:, 6] = np.sqrt(5.0) / 2 * (3 * z * z - 1.0)
    sh[:, 7] = np.sqrt(15.0) * x * z
    sh[:, 8] = np.sqrt(15.0) / 2 * (x * x - y * y)
    freqs = np.arange(1, NRAD + 1, dtype=np.float64)
    xr = np.clip(d, 1e-4, 1.0)[:, None].astype(np.float64)
    basis = (np.sqrt(2.0) * np.sin(freqs * np.pi * xr) / xr).astype(np.float32)
    cut = (0.5 * (np.cos(np.pi * np.clip(d, 0.0, 1.0)) + 1.0)).astype(np.float32)
    return (sh * cut[:, None]).astype(np.float32), basis


def partition_graph(receivers):
    import heapq
    deg = np.bincount(receivers, minlength=N_NODES)
    order = np.argsort(-deg, kind="stable")
    nbins = NCORES * NW
    load = np.zeros(nbins, np.int64)
    cnt = np.zeros(nbins, np.int64)
    owner = np.empty(N_NODES, np.int32)
    local = np.empty(N_NODES, np.int32)
    heap = [(0, b) for b in range(nbins)]
    heapq.heapify(heap)
    for n in order:
        while True:
            l, b = heapq.heappop(heap)
            if cnt[b] < 128:
                break
        owner[n] = b // NW
        local[n] = (b % NW) * 128 + cnt[b]
        cnt[b] += 1
        load[b] += deg[n]
        if cnt[b] < 128:
            heapq.heappush(heap, (int(load[b]), b))
    nodes_of = np.empty((NCORES, NPC), np.int64)
    for n in range(N_NODES):
        nodes_of[owner[n], local[n]] = n
    return owner, local, nodes_of, int(load.max())


def build_core_edges(receivers, owner, local, tpw):
    T = NW * tpw
    perm = np.full((NCORES, T * 128), -1, np.int64)
    for k in range(NCORES):
        eids = np.where(owner[receivers] == k)[0]
        lr = local[receivers[eids]]
        o = np.argsort(lr, kind="stable")
        eids, lr = eids[o], lr[o]
        w_of = lr // 128
        for w in range(NW):
            sel = eids[w_of == w]
            assert len(sel) <= tpw * 128, "tiles-per-window overflow"
            base = w * tpw * 128
            perm[k, base:base + len(sel)] = sel
    return perm


# ---------------- bass kernel builder ----------------
def build_layer_kernel(layer2, T, debug=False):
    import concourse.bass as bass
    import concourse.bacc as bacc
    import concourse.tile as tile
    import concourse.mybir as mybir
    from contextlib import ExitStack

    fp32 = mybir.dt.float32
    bf16 = mybir.dt.bfloat16
    AF = mybir.ActivationFunctionType
    ALU = mybir.AluOpType

    NPATH = 15 if layer2 else 3
    GCOLP = GCOLP_L2 if layer2 else GCOLP_L1
    W3COL = NPATH * C if layer2 else 576   # L1 w3 host-expanded to 9 kg blocks
    E_PAD = T * 128
    NG = T // BP
    assert T % BP == 0 and T % NW == 0
    tpw = T // NW

    import os as _os
    STAGE = int(_os.environ.get("KV2_STAGE", "5"))
    nc = bacc.Bacc("TRN2", target_bir_lowering=False)

    ftab = nc.dram_tensor("ftab", [N_NODES, GCOLP], bf16, kind="ExternalInput")
    sidx = nc.dram_tensor("sidx", [128, E_PAD // 16], mybir.dt.int16,
                          kind="ExternalInput")
    shp_d = nc.dram_tensor("shp", [128, T, 9], bf16, kind="ExternalInput")
    shq_d = nc.dram_tensor("shq", [128, T, NSHQ], fp32, kind="ExternalInput")
    basT_d = nc.dram_tensor("basisT", [NG, 8, BP * 128], bf16,
                            kind="ExternalInput")
    smat_d = nc.dram_tensor("smat", [128, T, 128], bf16, kind="ExternalInput")
    oldT_d = nc.dram_tensor("oldT", [64, 9 * NPC], fp32, kind="ExternalInput")
    w1_d = nc.dram_tensor("w1", [8, H], bf16, kind="ExternalInput")
    b1_d = nc.dram_tensor("b1", [H, 1], fp32, kind="ExternalInput")
    w2_d = nc.dram_tensor("w2", [H, H], bf16, kind="ExternalInput")
    b2_d = nc.dram_tensor("b2", [H, 1], fp32, kind="ExternalInput")
    w3_d = nc.dram_tensor("w3", [H, W3COL], bf16, kind="ExternalInput")
    lin_d = [nc.dram_tensor(f"lin{l}", [C, C], fp32, kind="ExternalInput")
             for l in range(3)]
    gw_d = [nc.dram_tensor(f"gw{l}", [C, C], fp32, kind="ExternalInput")
            for l in range(2)]
    gb_d = [nc.dram_tensor(f"gb{l}", [C, 1], fp32, kind="ExternalInput")
            for l in range(2)]
    newT_d = nc.dram_tensor("newT", [64, 9 * NPC], fp32,
                            kind="ExternalOutput")
    if debug:
        dbg_xg = nc.dram_tensor("dbg_xg", [128, BP, GCOLP], fp32,
                                kind="ExternalOutput")
        dbg_w = nc.dram_tensor("dbg_w", [128, BP, W3COL], fp32,
                               kind="ExternalOutput")
        dbg_msgs = nc.dram_tensor("dbg_msgs", [128, BP, F], fp32,
                                  kind="ExternalOutput")
        dbg_agg = nc.dram_tensor("dbg_agg", [128, NW, 640], fp32,
                                 kind="ExternalOutput")

    with tile.TileContext(nc) as tc, ExitStack() as ctx:
        consts = ctx.enter_context(tc.tile_pool(name="consts", bufs=1))
        idx_sb = consts.tile([128, E_PAD // 16], mybir.dt.int16)
        nc.sync.dma_start(idx_sb[:], sidx[:])
        w1_sb = consts.tile([8, H], bf16)
        nc.sync.dma_start(w1_sb[:], w1_d[:])
        b1_sb = consts.tile([H, 1], fp32)
        nc.sync.dma_start(b1_sb[:], b1_d[:])
        w2_sb = consts.tile([H, H], bf16)
        nc.sync.dma_start(w2_sb[:], w2_d[:])
        b2_sb = consts.tile([H, 1], fp32)
        nc.sync.dma_start(b2_sb[:], b2_d[:])
        w3_sb = consts.tile([H, W3COL], bf16)
        nc.sync.dma_start(w3_sb[:], w3_d[:])
        lin_sb = [consts.tile([C, C], fp32, name=f"lin{l}", tag=f"lin{l}")
                  for l in range(3)]
        for l in range(3):
            nc.sync.dma_start(lin_sb[l][:], lin_d[l][:])
        gw_sb = [consts.tile([C, C], fp32, name=f"gw{l}", tag=f"gw{l}")
                 for l in range(2)]
        gb_sb = [consts.tile([C, 1], fp32, name=f"gb{l}", tag=f"gb{l}")
                 for l in range(2)]
        for l in range(2):
            nc.sync.dma_start(gw_sb[l][:], gw_d[l][:])
            nc.sync.dma_start(gb_sb[l][:], gb_d[l][:])
        agg_sb = consts.tile([64, NW, 2, 5, 128], fp32)

        with ExitStack() as psctx:
            iop = psctx.enter_context(tc.tile_pool(name="iop", bufs=2))
            aggt = psctx.enter_context(tc.tile_pool(name="aggt", bufs=1))
            wp = psctx.enter_context(tc.tile_pool(name="wp", bufs=1))
            msgp = psctx.enter_context(tc.tile_pool(name="msgp", bufs=2))
            shbp = psctx.enter_context(tc.tile_pool(name="shbp", bufs=1))
            pp = psctx.enter_context(tc.tile_pool(name="pp", bufs=1))
            zjp = psctx.enter_context(tc.tile_pool(name="zjp", bufs=1))
            zgp = psctx.enter_context(tc.tile_pool(name="zgp", bufs=1))
            h_ps = psctx.enter_context(
                tc.tile_pool(name="h_ps", bufs=1, space="PSUM"))
            w_ps_pool = psctx.enter_context(
                tc.tile_pool(name="w_ps", bufs=2, space="PSUM"))
            agg_pool = psctx.enter_context(
                tc.tile_pool(name="agg_ps", bufs=2, space="PSUM"))

            agg_open = {}

            for g in range(NG):
                t0 = g * BP
                xg = iop.tile([128, BP, GCOLP], bf16, tag="xg")
                nc.gpsimd.dma_gather(
                    out_ap=xg[:],
                    in_ap=ftab[:],
                    idxs_ap=idx_sb[:, g * (BP * 8):(g + 1) * (BP * 8)],
                    num_idxs=BP * 128,
                    num_idxs_reg=BP * 128,
                    elem_size=GCOLP,
                )
                shp_t = iop.tile([128, BP, 9], bf16, tag="shp")
                nc.sync.dma_start(shp_t[:], shp_d[:, t0:t0 + BP, :])
                if layer2:
                    shq_t = iop.tile([128, BP, NSHQ], fp32, tag="shq")
                    nc.sync.dma_start(shq_t[:], shq_d[:, t0:t0 + BP, :])
                smt = iop.tile([128, BP, 128], bf16, tag="smt")
                nc.sync.dma_start(smt[:], smat_d[:, t0:t0 + BP, :])
                bas = iop.tile([8, BP * 128], bf16, tag="bas")
                nc.sync.dma_start(bas[:], basT_d[g, :, :])

                if STAGE < 2:
                    continue
                if not layer2:
                    # sh broadcast table for L1 (ScalarE, stride-0 copy)
                    shB = shbp.tile([128, BP, 9, C], bf16, tag="shB")
                    nc.scalar.copy(
                        shB[:],
                        shp_t[:, :, :, None].broadcast_to((128, BP, 9, C)))

                # ---- radial MLP (transposed; fused Silu) ----
                h1s = iop.tile([H, BP * 128], bf16, tag="h1s")
                h2s = iop.tile([H, BP * 128], bf16, tag="h2s")
                sg = iop.tile([H, BP * 128], bf16, tag="sg")
                for c0 in range(0, BP * 128, 512):
                    h1p = h_ps.tile([H, 512], fp32, tag="h")
                    nc.tensor.matmul(h1p[:], w1_sb[:],
                                     bas[:, c0:c0 + 512], start=True, stop=True)
                    nc.scalar.activation(sg[:, c0:c0 + 512], h1p[:],
                                         AF.Sigmoid, bias=b1_sb[:, 0:1])
                    nc.vector.scalar_tensor_tensor(
                        out=h1s[:, c0:c0 + 512], in0=h1p[:],
                        scalar=b1_sb[:, 0:1], in1=sg[:, c0:c0 + 512],
                        op0=ALU.add, op1=ALU.mult)
                for c0 in range(0, BP * 128, 512):
                    h2p = h_ps.tile([H, 512], fp32, tag="h")
                    nc.tensor.matmul(h2p[:], w2_sb[:],
                                     h1s[:, c0:c0 + 512], start=True, stop=True)
                    nc.scalar.activation(sg[:, c0:c0 + 512], h2p[:],
                                         AF.Sigmoid, bias=b2_sb[:, 0:1])
                    nc.vector.scalar_tensor_tensor(
                        out=h2s[:, c0:c0 + 512], in0=h2p[:],
                        scalar=b2_sb[:, 0:1], in1=sg[:, c0:c0 + 512],
                        op0=ALU.add, op1=ALU.mult)

                # ---- per-tile edge weights w = h2s_t.T @ W3 (bf16 out) ----
                # one 1-bank PSUM tile per 512-col chunk, double-buffered:
                # tile t+1's matmul overlaps tile t's PSUM->SBUF copy
                w_sb = wp.tile([128, BP, W3COL], bf16, tag="wsb")
                for bt in range(BP):
                    for c0 in range(0, W3COL, 512):
                        c1 = min(c0 + 512, W3COL)
                        w_ps = w_ps_pool.tile([128, 512], fp32, tag="wps")
                        nc.tensor.matmul(w_ps[:, 0:c1 - c0],
                                         h2s[:, bt * 128:(bt + 1) * 128],
                                         w3_sb[:, c0:c1], start=True, stop=True)
                        nc.scalar.copy(w_sb[:, bt, c0:c1], w_ps[:, 0:c1 - c0])

                if STAGE < 3:
                    continue
                # ---- tensor product ----
                msgs = msgp.tile([128, BP, F], bf16, tag="msgs")
                if layer2:
                    _emit_tp_l2(nc, ALU, xg, w_sb, shq_t, msgs, pp, zjp, zgp)
                else:
                    _emit_tp_l1(nc, ALU, xg, w_sb, shB, msgs, pp)

                if debug and g == 0:
                    dxg = pp.tile([128, BP, GCOLP], fp32, tag="dxg")
                    nc.vector.tensor_copy(out=dxg[:], in_=xg[:])
                    nc.sync.dma_start(dbg_xg[:], dxg[:])
                    dw = pp.tile([128, BP, W3COL], fp32, tag="dw")
                    nc.vector.tensor_copy(out=dw[:], in_=w_sb[:])
                    nc.sync.dma_start(dbg_w[:], dw[:])
                    dmg = pp.tile([128, BP, F], fp32, tag="dmg")
                    nc.vector.tensor_copy(out=dmg[:], in_=msgs[:])
                    nc.sync.dma_start(dbg_msgs[:], dmg[:])

                if STAGE < 4:
                    continue
                # ---- segment sum: paired-kg bf16 matmuls, PSUM-accumulated ----
                for bt in range(BP):
                    t = t0 + bt
                    w, t_in_w = t // tpw, t % tpw
                    if w not in agg_open:
                        agg_open[w] = agg_pool.tile([128, 640], fp32,
                                                    name="aggps", tag="aggps")
                    ps = agg_open[w]
                    first, last = t_in_w == 0, t_in_w == tpw - 1
                    for pair in range(5):
                        c0 = pair * 128
                        cw = 128 if pair < 4 else 64
                        nc.tensor.matmul(
                            ps[0:cw, c0:c0 + 128],
                            msgs[:, bt, c0:c0 + cw],
                            smt[:, bt, :],
                            start=first and pair in (0, 4),
                            stop=last, skip_group_check=True)
                    if last:
                        aps = agg_open.pop(w)
                        afp = aggt.tile([128, 640], fp32, tag="afp")
                        nc.scalar.copy(afp[:, 0:512], aps[:, 0:512])
                        nc.scalar.copy(afp[0:64, 512:640], aps[0:64, 512:640])
                        nc.sync.dma_start(
                            agg_sb[:, w, 0, :, :],
                            afp[0:64, :].rearrange("p (q n) -> p q n", q=5))
                        nc.sync.dma_start(
                            agg_sb[:, w, 1, 0:4, :],
                            afp[64:128, 0:512].rearrange("p (q n) -> p q n",
                                                         q=4))



        # ---------------- per-window node update ----------------
        if STAGE < 5:
            with ExitStack() as upctx:
                upt = upctx.enter_context(tc.tile_pool(name="upt", bufs=2))
                for w in range(NW):
                    tmp = upt.tile([64, 9, 128], fp32, tag="pass")
                    nc.sync.dma_start(
                        tmp[:], oldT_d[:, :].rearrange(
                            "p (q n) -> p q n",
                            q=9)[:, :, w * 128:(w + 1) * 128])
                    nc.sync.dma_start(
                        newT_d[:, :].rearrange(
                            "p (q n) -> p q n",
                            q=9)[:, :, w * 128:(w + 1) * 128],
                        tmp[:])
            nc.compile()
            return nc
        with ExitStack() as upctx:
            y_pool = upctx.enter_context(
                tc.tile_pool(name="y_ps", bufs=2, space="PSUM"))
            g_pool = upctx.enter_context(
                tc.tile_pool(name="g_ps", bufs=2, space="PSUM"))
            upt = upctx.enter_context(tc.tile_pool(name="upt", bufs=2))
            for w in range(NW):
                oldw = upt.tile([64, 9, 128], fp32, tag="oldw")
                nc.sync.dma_start(
                    oldw[:], oldT_d[:, :].rearrange(
                        "p (q n) -> p q n", q=9)[:, :, w * 128:(w + 1) * 128])
                neww = upt.tile([64, 9, 128], fp32, tag="neww")
                y_ps = y_pool.tile([64, 9 * 128], fp32, tag="yps")
                for kg in range(9):
                    l = 0 if kg == 0 else (1 if kg <= 3 else 2)
                    nc.tensor.matmul(
                        y_ps[:, kg * 128:(kg + 1) * 128],
                        lin_sb[l][:],
                        agg_sb[:, w, kg % 2, kg // 2, :],
                        start=kg in (0, 4, 8), stop=True,
                        skip_group_check=True)
                y0g = upt.tile([C, 128], fp32, tag="y0g")
                nc.scalar.activation(y0g[:], y_ps[:, 0:128], AF.Sigmoid)
                y0s = upt.tile([C, 128], fp32, tag="y0s")
                nc.vector.tensor_tensor(out=y0s[:], in0=y_ps[:, 0:128],
                                        in1=y0g[:], op=ALU.mult)
                nc.vector.tensor_tensor(out=neww[:, 0, :], in0=y0s[:],
                                        in1=oldw[:, 0, :], op=ALU.add)
                g_ps = g_pool.tile([C, 2, 128], fp32, tag="gps")
                for l in (1, 2):
                    nc.tensor.matmul(g_ps[:, l - 1, :], gw_sb[l - 1][:],
                                     neww[:, 0, :], start=(l == 1), stop=True,
                                     skip_group_check=True)
                gts = upt.tile([C, 2, 128], fp32, tag="gts")
                for l in (1, 2):
                    nc.scalar.activation(gts[:, l - 1, :], g_ps[:, l - 1, :],
                                         AF.Sigmoid, bias=gb_sb[l - 1][:, 0:1])
                gy = upt.tile([C, 8, 128], fp32, tag="gy")
                nc.vector.tensor_tensor(
                    out=gy[:, 0:3, :],
                    in0=y_ps[:].rearrange("p (q n) -> p q n", q=9)[:, 1:4, :],
                    in1=gts[:, 0:1, :].broadcast_to((C, 3, 128)),
                    op=ALU.mult)
                nc.vector.tensor_tensor(
                    out=gy[:, 3:8, :],
                    in0=y_ps[:].rearrange("p (q n) -> p q n", q=9)[:, 4:9, :],
                    in1=gts[:, 1:2, :].broadcast_to((C, 5, 128)),
                    op=ALU.mult)
                nc.vector.tensor_tensor(out=neww[:, 1:9, :], in0=gy[:],
                                        in1=oldw[:, 1:9, :], op=ALU.add)
                nc.sync.dma_start(
                    newT_d[:, :].rearrange("p (q n) -> p q n",
                                           q=9)[:, :, w * 128:(w + 1) * 128],
                    neww[:])

    nc.compile()
    return nc


def _emit_tp_l2(nc, ALU, xg, w_sb, shq_t, msgs, pp, zjp, zgp):
    """P products, cg-folded zjQ scalings, FMA as adds (+8 ratio stts).

    All on DVE: single in-order queue, no cross-engine ping-pong; GpSimd
    has ~9us fixed cost per elementwise op and ScalarE cannot multiply
    two tensors, so the TP bulk lives here.
    """
    import concourse.mybir as mybir
    bf16 = mybir.dt.bfloat16
    fp32 = mybir.dt.float32

    GP_JS = ()          # GpSimd streams ~0.4 elem/cyc and zjG bufs=1
                        # serialized groups -> ~19us DVE stall per group;
                        # cheaper to keep all zjq on DVE (nnz-only now)
    EXPC = 8            # ping-pong: 2 x 8x8x32 fp32 = 16KB
    P0 = pp.tile([128, BP, SCHED2_L2[0]["nblk"] * C], bf16, name="P0", tag="P0")
    P12 = pp.tile([128, BP, SCHED2_L2[2]["nblk"] * C], bf16,
                  name="P12", tag="P12")
    zj12 = zjp.tile([128, BP, SCHED2_L2[2]["nblk"] * C], bf16,
                    name="zj12", tag="zj12")
    exps = [zgp.tile([128, BP, EXPC, C // 2], fp32, name=f"exp{i}",
                     tag=f"exp{i}") for i in range(2)]
    ppg = [0]

    def emit_zjq(eng, dst, P, qoff, segs):
        # shq host-packed as fp32 = (v,v) bf16 pair: ScalarE broadcast-
        # expands at half the element count, DVE bitcasts back to bf16 and
        # multiplies with unit-stride in1 -> 2x_1P mode. 4-deep ping-pong
        # so expansion runs ahead of consumption across FMA phases.
        chunks, cur, used = [], [], 0
        for (z0, L) in segs:
            while L > 0:
                if used == EXPC:
                    chunks.append(cur)
                    cur, used = [], 0
                take = min(L, EXPC - used)
                cur.append((z0, take, used))
                z0, L, used = z0 + take, L - take, used + take
        if cur:
            chunks.append(cur)
        for ch in chunks:
            exp = exps[ppg[0]]
            ppg[0] = (ppg[0] + 1) % 2
            for (z0, L, eo) in ch:
                nc.scalar.copy(
                    exp[:, :, eo:eo + L, :],
                    shq_t[:, :, qoff + z0:qoff + z0 + L, None].broadcast_to(
                        (128, BP, L, C // 2)))
            expb = exp[:].bitcast(mybir.dt.bfloat16)
            for (z0, L, eo) in ch:
                eng.tensor_tensor(
                    out=dst[:].rearrange("p b (n c) -> p b n c",
                                         c=C)[:, :, z0:z0 + L, :],
                    in0=P[:].rearrange("p b (n c) -> p b n c",
                                       c=C)[:, :, z0:z0 + L, :],
                    in1=expb[:, :, eo:eo + L, :],
                    op=ALU.mult)

    def emit_fma(sch, j, zj):
        for (jj, z0, k0, L) in sch["runs_add"]:
            if jj != j:
                continue
            nc.vector.tensor_tensor(
                out=msgs[:, :, k0 * C:(k0 + L) * C],
                in0=zj[:, :, z0 * C:(z0 + L) * C],
                in1=msgs[:, :, k0 * C:(k0 + L) * C],
                op=ALU.add)
        for (jj, z, k, ratio) in sch["runs_stt"]:
            if jj != j:
                continue
            nc.vector.scalar_tensor_tensor(
                out=msgs[:, :, k * C:(k + 1) * C],
                in0=zj[:, :, z * C:(z + 1) * C],
                scalar=float(ratio),
                in1=msgs[:, :, k * C:(k + 1) * C],
                op0=ALU.mult, op1=ALU.add)

    for gi, sch in enumerate(SCHED2_L2):
        nblk = sch["nblk"]
        P = P0 if gi == 0 else P12
        for (p, ig0, ni, z0) in sch["pops"]:
            nc.vector.tensor_tensor(
                out=P[:].rearrange("p b (n c) -> p b n c",
                                   c=C)[:, :, z0:z0 + ni, :],
                in0=xg[:].rearrange("p b (n c) -> p b n c",
                                    c=C)[:, :, ig0:ig0 + ni, :],
                in1=w_sb[:, :, None, p * C:(p + 1) * C].broadcast_to(
                    (128, BP, ni, C)),
                op=ALU.mult)
        if gi > 0:
            # kick GpSimd's share first so it overlaps DVE's other j's
            for (j, qoff, segs) in sch["zjq"]:
                if j in GP_JS:
                    emit_zjq(nc.gpsimd, zjG[j], P, qoff, segs)
        for (j, qoff, segs) in sch["zjq"]:
            if gi > 0 and j in GP_JS:
                continue
            dst = msgs if gi == 0 else zj12
            emit_zjq(nc.vector, dst, P, qoff, segs)
            if gi == 0:
                continue
            emit_fma(sch, j, zj12)
        if gi > 0:
            for j in GP_JS:
                if any(jj == j for (jj, _, _) in sch["zjq"]):
                    emit_fma(sch, j, zjG[j])


def _emit_tp_l1(nc, ALU, xg, w_sb, shB, msgs, pp):
    """msgs[k] = x * w'_k * sh_k ; w' host-expanded+cg-folded to 9 kg blocks."""
    import concourse.mybir as mybir
    bf16 = mybir.dt.bfloat16
    tmp = pp.tile([128, BP, 9, C], bf16, name="l1tmp", tag="l1tmp")
    nc.vector.tensor_tensor(
        out=tmp[:],
        in0=w_sb[:].rearrange("p b (n c) -> p b n c", c=C),
        in1=shB[:],
        op=ALU.mult)
    nc.vector.tensor_tensor(
        out=msgs[:].rearrange("p b (n c) -> p b n c", c=C),
        in0=tmp[:],
        in1=xg[:, :, None, 0:C].broadcast_to((128, BP, 9, C)),
        op=ALU.mult)


# ---------------- host orchestration ----------------
def _chunked_T(feats_own):
    """[NPC, 576] -> kg-blocked transposed [64, 9*NPC]."""
    out = np.empty((64, 9 * NPC), np.float32)
    for kg in range(9):
        out[:, kg * NPC:(kg + 1) * NPC] = feats_own[:, kg * 64:(kg + 1) * 64].T
    return out


def _unchunk_T(newT):
    """[64, 9*NPC] -> [NPC, 576]."""
    out = np.empty((NPC, 576), np.float32)
    for kg in range(9):
        out[:, kg * 64:(kg + 1) * 64] = newT[:, kg * NPC:(kg + 1) * NPC].T
    return out


_CACHE = {}


def _prep(positions, senders, receivers):
    key = (senders.tobytes(), receivers.tobytes(), positions.tobytes())
    if _CACHE.get("key") == key:
        return _CACHE["val"]
    sh_eff, basis = edge_geometry(positions, senders, receivers)
    owner, local, nodes_of, _ = partition_graph(receivers)
    deg_bin = np.zeros(NCORES * NW, np.int64)
    np.add.at(deg_bin, owner[receivers] * NW + local[receivers] // 128, 1)
    tpw = (int(deg_bin.max()) + 127) // 128
    T = NW * tpw
    assert T % BP == 0
    perm = build_core_edges(receivers, owner, local, tpw)

    valid = perm >= 0
    eg = np.where(valid, perm, 0)
    snd = np.where(valid, senders[eg], 0).astype(np.int16)      # [NC, T*128]
    shp_e = sh_eff[eg] * valid[..., None]                        # [NC, T*128, 9]
    bas_e = basis[eg] * valid[..., None]                         # [NC, T*128, 8]
    lr = np.where(valid, local[receivers[eg]], 0)

    NG = T // BP
    inv = np.float32(1.0 / np.sqrt(AVG_NN))
    sidx = np.empty((NCORES, 128, T * 128 // 16), np.int16)
    shp_h = np.empty((NCORES, 128, T, 9), BF16)
    shq_h = np.empty((NCORES, 128, T, NSHQ), np.float32)
    jcols = np.array([j for (j, cg) in SHQ_COLS])
    cgv = np.array([cg for (j, cg) in SHQ_COLS], np.float32)
    bas_h = np.empty((NCORES, NG, 8, BP * 128), BF16)
    smat_h = np.zeros((NCORES, 128, T, 128), BF16)
    for k in range(NCORES):
        s = snd[k].reshape(T * 8, 16)
        sidx[k] = np.tile(s.T, (8, 1))
        shp_f = shp_e[k].reshape(T, 128, 9).transpose(1, 0, 2)
        shp_h[k] = shp_f.astype(BF16)
        # fp32-packed (v, v) bf16 pair so on-chip broadcast-expansion moves
        # half the elements (bitcast back to bf16 at the consumer)
        u = (shp_f[:, :, jcols] * cgv[None, None, :]).astype(
            BF16).view(np.uint16).astype(np.uint32)
        shq_h[k] = ((u << 16) | u).view(np.float32)
        bas_h[k] = bas_e[k].reshape(NG, BP * 128, 8).transpose(0, 2, 1).astype(BF16)
        v = valid[k]
        e_slots = np.arange(T * 128)
        p_, t_ = e_slots % 128, e_slots // 128
        cols = lr[k] - (t_ // tpw) * 128
        ok = v & (cols >= 0) & (cols < 128)
        sm = np.zeros((128, T, 128), np.float32)
        sm[p_[ok], t_[ok], cols[ok]] = inv
        smat_h[k] = sm.astype(BF16)
    val = dict(T=T, NG=NG, tpw=tpw, nodes_of=nodes_of, sidx=sidx,
               shp_h=shp_h, bas_h=bas_h, smat_h=smat_h, shq_h=shq_h)
    _CACHE["key"], _CACHE["val"] = key, val
    return val


EXEC_NS = []


def _run_layer(nc, pre, ftab, oldT_by_core, lw):
    import os
    from concourse.bass_utils import run_bass_kernel_spmd
    in_maps = []
    for k in range(NCORES):
        m = dict(ftab=ftab,
                 sidx=pre["sidx"][k],
                 shp=pre["shp_h"][k],
                 shq=pre["shq_h"][k],
                 basisT=pre["bas_h"][k],
                 smat=pre["smat_h"][k],
                 oldT=oldT_by_core[k],
                 w1=lw["w1"], b1=lw["b1"], w2=lw["w2"], b2=lw["b2"],
                 w3=lw["w3"], lin0=lw["lin"][0], lin1=lw["lin"][1],
                 lin2=lw["lin"][2], gw0=lw["gw"][0], gw1=lw["gw"][1],
                 gb0=lw["gb"][0], gb1=lw["gb"][1])
        in_maps.append(m)
    trace = bool(os.environ.get("KERNEL_TRACE"))
    res = run_bass_kernel_spmd(nc, in_maps, list(range(NCORES)), trace=trace,
                               trace_cores=list(range(NCORES)) if trace else None)
    if trace and res.exec_time_ns is not None:
        EXEC_NS.append(res.exec_time_ns)
    return [res.results[k]["newT"] for k in range(NCORES)]


def _layer_weights(inputs, i, layer2):
    f32 = np.float32
    w3 = np.ascontiguousarray(inputs["mlp_w3"][i], f32)
    if layer2:
        w3p = w3.astype(BF16)
    else:
        w3p = np.empty((H, 576), BF16)
        for kg in range(9):
            p = L1_PATH_OF_K[kg]
            w3p[:, kg * C:(kg + 1) * C] = (
                w3[:, p * C:(p + 1) * C] * L1_CG_OF_K[kg]).astype(BF16)
    return dict(
        w1=np.ascontiguousarray(inputs["mlp_w1"][i], f32).astype(BF16),
        b1=np.ascontiguousarray(inputs["mlp_b1"][i], f32).reshape(H, 1),
        w2=np.ascontiguousarray(inputs["mlp_w2"][i], f32).astype(BF16),
        b2=np.ascontiguousarray(inputs["mlp_b2"][i], f32).reshape(H, 1),
        w3=w3p,
        lin=[np.ascontiguousarray(inputs["lin_self"][i, l], f32)
             for l in range(3)],
        gw=[np.ascontiguousarray(inputs["gate_w"][i, l], f32)
            for l in range(2)],
        gb=[np.ascontiguousarray(inputs["gate_b"][i, l], f32).reshape(C, 1)
            for l in range(2)],
    )


_KERNEL_CACHE = {}


def _get_kernels(T):
    if T not in _KERNEL_CACHE:
        _KERNEL_CACHE[T] = (build_layer_kernel(False, T),
                            build_layer_kernel(True, T))
    return _KERNEL_CACHE[T]


def _pack_ftab(table, ncols):
    out = np.zeros((N_NODES, ncols), BF16)
    used = min(ncols, table.shape[1])
    out[:, :used] = table[:, :used].astype(BF16)
    return out


def kernel(**inputs):
    positions = np.asarray(inputs["positions"], np.float32)
    species = np.asarray(inputs["species"]).astype(np.int64)
    senders = np.asarray(inputs["senders"]).astype(np.int64)
    receivers = np.asarray(inputs["receivers"]).astype(np.int64)

    pre = _prep(positions, senders, receivers)
    T = pre["T"]
    nc1, nc2 = _get_kernels(T)
    nodes_of = pre["nodes_of"]

    # initial features: x0 from species embedding (host; tiny)
    x0 = (np.asarray(inputs["embed"], np.float32)[species]
          @ np.asarray(inputs["w_proj"], np.float32))          # [N, 64]
    table = np.zeros((N_NODES, F), np.float32)
    table[:, 0:C] = x0

    # ---- layer 1 ----
    oldT = [_chunked_T(table[nodes_of[k]]) for k in range(NCORES)]
    lw = _layer_weights(inputs, 0, False)
    newT = _run_layer(nc1, pre, _pack_ftab(table, GCOLP_L1), oldT, lw)

    table2 = np.empty((N_NODES, F), np.float32)
    for k in range(NCORES):
        table2[nodes_of[k]] = _unchunk_T(newT[k])

    # ---- layer 2 ----
    lw = _layer_weights(inputs, 1, True)
    newT2 = _run_layer(nc2, pre, _pack_ftab(table2, GCOLP_L2), newT, lw)

    table3 = np.empty((N_NODES, F), np.float32)
    for k in range(NCORES):
        table3[nodes_of[k]] = _unchunk_T(newT2[k])

    # ---- output: reorder component-major -> reference layout + alpha ----
    t3 = table3.reshape(N_NODES, 9, C)
    out = np.empty((N_NODES, F), np.float32)
    out[:, 0:64] = t3[:, 0]
    out[:, 64:256] = (0.5 * t3[:, 1:4]).transpose(0, 2, 1).reshape(N_NODES, 192)
    out[:, 256:576] = (0.25 * t3[:, 4:9]).transpose(0, 2, 1).reshape(N_NODES, 320)
    return out



# revision 24
# speedup vs baseline: 1.3513x; 1.0070x over previous
"""NequIP GNN message-passing kernel for 8 Trainium2 NeuronCores — v2.

Receiver-sharded graph parallelism (per sharding hint): host LPT-assigns the
8192 nodes to 64 (core, window) bins of 128, each core owns 8 windows = 1024
nodes + their in-edges, sorted by window, padded to 128-edge tiles.

v2 device pipeline (vs v1): bf16 edge pipeline end-to-end with fp32 PSUM
accumulation; per-edge-scalar stages batched across 8-tile groups via
stride-0 broadcast access patterns; P-products collapsed per-path (w operand
broadcast over l1 components); CG-FMA stage merged into diagonal runs with
immediate scalars; segment-sum via paired-kg bf16 matmuls; radial MLP with
fused Silu activations; elementwise work split across Vector/GpSimd/Scalar.
"""
import math
import numpy as np
import ml_dtypes

BF16 = ml_dtypes.bfloat16

# ---------------- model constants ----------------
N_NODES, N_EDGES = 8192, 131072
C, H, NRAD = 64, 64, 8
R_MAX, AVG_NN = 5.0, 16.0
NCORES, NPC = 8, 1024
NW = NPC // 128
F = 9 * C
LS = (0, 1, 2)
PATHS = [(l1, l2, l3) for l1 in LS for l2 in LS for l3 in LS
         if abs(l1 - l2) <= l3 <= l1 + l2]
LOFF = {0: 0, 1: 1, 2: 4}
J_OF_L2 = {0: [0], 1: [1, 2, 3], 2: [4, 5, 6, 7, 8]}
BP = 8                      # tiles per group (batched in op free dims)
GCOLP_L2 = 640              # padded gather row (bf16): 1280B % 256 == 0
GCOLP_L1 = 128              # 256B % 256 == 0


# ---------------- real Clebsch-Gordan coefficients ----------------
def _cg_scalar(j1, m1, j2, m2, j3, m3):
    f = math.factorial
    if m1 + m2 != m3:
        return 0.0
    pre = ((2*j3+1) * f(j1+j2-j3) * f(j1-j2+j3) * f(-j1+j2+j3)
           / f(j1+j2+j3+1)) ** 0.5
    pre *= (f(j1+m1)*f(j1-m1)*f(j2+m2)*f(j2-m2)*f(j3+m3)*f(j3-m3)) ** 0.5
    s = 0.0
    for k in range(max(0, j2-j3-m1, j1+m2-j3), min(j1+j2-j3, j1-m1, j2+m2)+1):
        s += (-1)**k / (f(k)*f(j1+j2-j3-k)*f(j1-m1-k)
                        * f(j2+m2-k)*f(j3-j2+m1+k)*f(j3-j1-m2+k))
    return pre * s


def _U_real(l):
    U = np.zeros((2*l+1, 2*l+1), dtype=complex)
    s2 = 2 ** -0.5
    for m in range(-l, l+1):
        if m > 0:
            U[m+l, m+l] = (-1)**m * s2
            U[m+l, -m+l] = s2
        elif m == 0:
            U[l, l] = 1.0
        else:
            U[m+l, m+l] = 1j*s2
            U[m+l, -m+l] = -1j*(-1)**(-m)*s2
    return U


def _real_cg(l1, l2, l3):
    Cc = np.zeros((2*l1+1, 2*l2+1, 2*l3+1))
    for i1, m1 in enumerate(range(-l1, l1+1)):
        for i2, m2 in enumerate(range(-l2, l2+1)):
            m3 = m1 + m2
            if abs(m3) <= l3:
                Cc[i1, i2, m3+l3] = _cg_scalar(l1, m1, l2, m2, l3, m3)
    U1, U2, U3 = _U_real(l1), _U_real(l2), _U_real(l3)
    W = np.einsum('ia,jb,kc,abc->ijk', U1.conj(), U2.conj(), U3,
                  Cc.astype(complex))
    W = W.real if np.linalg.norm(W.real) >= np.linalg.norm(W.imag) else W.imag
    W = W / np.linalg.norm(W) * (2*l3+1) ** 0.5
    return np.asarray(W, dtype=np.float64)


CGS = [_real_cg(*p) for p in PATHS]


def build_schedule_l2():
    """Static TP structure for layer 2, grouped by l2.

    Per group: nblk, pops=[(path, ig0, ni, z0)], jlist, runs=[(j,z0,k0,L,cg)].
    """
    scheds = []
    for l2 in (0, 1, 2):
        ps = [p for p in range(len(PATHS)) if PATHS[p][1] == l2]
        blocks, block_of, pops = [], {}, []
        for p in ps:
            l1 = PATHS[p][0]
            ni = 2 * l1 + 1
            pops.append((p, LOFF[l1], ni, len(blocks)))
            for i in range(ni):
                block_of[(p, i)] = len(blocks)
                blocks.append((p, i))
        tset = set()
        for p in ps:
            l1, _, l3 = PATHS[p]
            cg = CGS[p]
            for i in range(2*l1+1):
                for j in range(2*l2+1):
                    for k in range(2*l3+1):
                        v = cg[i, j, k]
                        if abs(v) > 1e-12:
                            tset.add((LOFF[l2]+j, block_of[(p, i)],
                                      LOFF[l3]+k, round(float(v), 9)))
        runs, consumed = [], set()
        for t in sorted(tset):
            if t in consumed:
                continue
            j, z, k, cgv = t
            L = 0
            while (j, z+L, k+L, cgv) in tset and (j, z+L, k+L, cgv) not in consumed:
                consumed.add((j, z+L, k+L, cgv))
                L += 1
            runs.append((j, z, k, L, cgv))
        scheds.append(dict(l2=l2, nblk=len(blocks), pops=pops,
                           jlist=J_OF_L2[l2], runs=runs))
    return scheds


SCHED_L2 = build_schedule_l2()


def build_schedule2():
    """cg folded into per-(j,z) host scalars; FMA mostly tensor adds.

    Per group: nblk, pops, zjq=[(j, qoff)], runs_add=[(j,z0,k0,L)],
    runs_stt=[(j,z,k,ratio)]. qoff = column offset of (j,*) block in the
    concatenated shq table [128, T, 179]. cgfold[qoff+z] = cg of the primary
    (smallest-k) triple of (j,z); 0 for unused (z,j).
    """
    scheds, qoff, cgcols = [], 0, []
    for gi, sch in enumerate(SCHED_L2):
        nblk = sch["nblk"]
        tset = set()
        for (j, z, k, L, cg) in sch["runs"]:
            for i in range(L):
                tset.add((j, z + i, k + i, cg))
        per_jz = {}
        for (j, z, k, cg) in tset:
            per_jz.setdefault((j, z), []).append((k, cg))
        primary, runs_stt = set(), []
        cgf = {}
        for (j, z), ks in per_jz.items():
            ks.sort()
            k0, cg0 = ks[0]
            primary.add((j, z, k0))
            cgf[(j, z)] = cg0
            for (k1, cg1) in ks[1:]:
                runs_stt.append((j, z, k1, cg1 / cg0))
        runs_add, consumed = [], set()
        for t in sorted(primary):
            if t in consumed:
                continue
            j, z, k = t
            L = 0
            while (j, z + L, k + L) in primary and (j, z + L, k + L) not in consumed:
                consumed.add((j, z + L, k + L))
                L += 1
            runs_add.append((j, z, k, L))
        zjq = []
        for j in sch["jlist"]:
            # contiguous segments of z columns actually used by (j, z):
            # skips the ~28% of dense columns with cg == 0
            used = sorted(z for (jj, z) in cgf if jj == j)
            segs, s0, prev = [], None, None
            for z in used:
                if s0 is None:
                    s0 = prev = z
                elif z == prev + 1:
                    prev = z
                else:
                    segs.append((s0, prev - s0 + 1))
                    s0 = prev = z
            if s0 is not None:
                segs.append((s0, prev - s0 + 1))
            zjq.append((j, qoff, segs))
            for z in range(nblk):
                cgcols.append((j, cgf.get((j, z), 0.0)))
            qoff += nblk
        scheds.append(dict(nblk=nblk, pops=sch["pops"], zjq=zjq,
                           runs_add=runs_add, runs_stt=runs_stt))
    return scheds, cgcols


SCHED2_L2, SHQ_COLS = build_schedule2()
NSHQ = len(SHQ_COLS)

# layer-1 per-k path and cg (x is scalar-only: paths (0,l,l), j == k)
L1_PATH_OF_K = [0] + [1]*3 + [2]*5
L1_CG_OF_K = []
for _k in range(9):
    _p = L1_PATH_OF_K[_k]
    _l = PATHS[_p][2]
    _m = _k - LOFF[_l]
    L1_CG_OF_K.append(float(CGS[_p][0, _m, _m]))


# ---------------- host-side graph preprocessing ----------------
def edge_geometry(positions, senders, receivers):
    rel = (positions[receivers] - positions[senders]) / R_MAX
    d = np.linalg.norm(rel, axis=-1)
    u = rel / np.maximum(d, 1e-6)[:, None]
    x, y, z = u[:, 0], u[:, 1], u[:, 2]
    sh = np.empty((len(d), 9), np.float32)
    sh[:, 0] = 1.0
    sh[:, 1] = np.sqrt(3.0) * y
    sh[:, 2] = np.sqrt(3.0) * z
    sh[:, 3] = np.sqrt(3.0) * x
    sh[:, 4] = np.sqrt(15.0) * x * y
    sh[:, 5] = np.sqrt(15.0) * y * z
    sh[:, 6] = np.sqrt(5.0) / 2 * (3 * z * z - 1.0)
    sh[:, 7] = np.sqrt(15.0) * x * z
    sh[:, 8] = np.sqrt(15.0) / 2 * (x * x - y * y)
    freqs = np.arange(1, NRAD + 1, dtype=np.float64)
    xr = np.clip(d, 1e-4, 1.0)[:, None].astype(np.float64)
    basis = (np.sqrt(2.0) * np.sin(freqs * np.pi * xr) / xr).astype(np.float32)
    cut = (0.5 * (np.cos(np.pi * np.clip(d, 0.0, 1.0)) + 1.0)).astype(np.float32)
    return (sh * cut[:, None]).astype(np.float32), basis


def partition_graph(receivers):
    import heapq
    deg = np.bincount(receivers, minlength=N_NODES)
    order = np.argsort(-deg, kind="stable")
    nbins = NCORES * NW
    load = np.zeros(nbins, np.int64)
    cnt = np.zeros(nbins, np.int64)
    owner = np.empty(N_NODES, np.int32)
    local = np.empty(N_NODES, np.int32)
    heap = [(0, b) for b in range(nbins)]
    heapq.heapify(heap)
    for n in order:
        while True:
            l, b = heapq.heappop(heap)
            if cnt[b] < 128:
                break
        owner[n] = b // NW
        local[n] = (b % NW) * 128 + cnt[b]
        cnt[b] += 1
        load[b] += deg[n]
        if cnt[b] < 128:
            heapq.heappush(heap, (int(load[b]), b))
    nodes_of = np.empty((NCORES, NPC), np.int64)
    for n in range(N_NODES):
        nodes_of[owner[n], local[n]] = n
    return owner, local, nodes_of, int(load.max())


def build_core_edges(receivers, owner, local, tpw):
    T = NW * tpw
    perm = np.full((NCORES, T * 128), -1, np.int64)
    for k in range(NCORES):
        eids = np.where(owner[receivers] == k)[0]
        lr = local[receivers[eids]]
        o = np.argsort(lr, kind="stable")
        eids, lr = eids[o], lr[o]
        w_of = lr // 128
        for w in range(NW):
            sel = eids[w_of == w]
            assert len(sel) <= tpw * 128, "tiles-per-window overflow"
            base = w * tpw * 128
            perm[k, base:base + len(sel)] = sel
    return perm


# ---------------- bass kernel builder ----------------
def build_layer_kernel(layer2, T, debug=False):
    import concourse.bass as bass
    import concourse.bacc as bacc
    import concourse.tile as tile
    import concourse.mybir as mybir
    from contextlib import ExitStack

    fp32 = mybir.dt.float32
    bf16 = mybir.dt.bfloat16
    AF = mybir.ActivationFunctionType
    ALU = mybir.AluOpType

    NPATH = 15 if layer2 else 3
    GCOLP = GCOLP_L2 if layer2 else GCOLP_L1
    W3COL = NPATH * C if layer2 else 576   # L1 w3 host-expanded to 9 kg blocks
    E_PAD = T * 128
    NG = T // BP
    assert T % BP == 0 and T % NW == 0
    tpw = T // NW

    import os as _os
    STAGE = int(_os.environ.get("KV2_STAGE", "5"))
    nc = bacc.Bacc("TRN2", target_bir_lowering=False)

    ftab = nc.dram_tensor("ftab", [N_NODES, GCOLP], bf16, kind="ExternalInput")
    sidx = nc.dram_tensor("sidx", [128, E_PAD // 16], mybir.dt.int16,
                          kind="ExternalInput")
    shp_d = nc.dram_tensor("shp", [128, T, 9], bf16, kind="ExternalInput")
    shq_d = nc.dram_tensor("shq", [128, T, NSHQ], fp32, kind="ExternalInput")
    basT_d = nc.dram_tensor("basisT", [NG, 8, BP * 128], bf16,
                            kind="ExternalInput")
    smat_d = nc.dram_tensor("smat", [128, T, 128], bf16, kind="ExternalInput")
    oldT_d = nc.dram_tensor("oldT", [64, 9 * NPC], fp32, kind="ExternalInput")
    w1_d = nc.dram_tensor("w1", [8, H], bf16, kind="ExternalInput")
    b1_d = nc.dram_tensor("b1", [H, 1], fp32, kind="ExternalInput")
    w2_d = nc.dram_tensor("w2", [H, H], bf16, kind="ExternalInput")
    b2_d = nc.dram_tensor("b2", [H, 1], fp32, kind="ExternalInput")
    w3_d = nc.dram_tensor("w3", [H, W3COL], bf16, kind="ExternalInput")
    lin_d = [nc.dram_tensor(f"lin{l}", [C, C], fp32, kind="ExternalInput")
             for l in range(3)]
    gw_d = [nc.dram_tensor(f"gw{l}", [C, C], fp32, kind="ExternalInput")
            for l in range(2)]
    gb_d = [nc.dram_tensor(f"gb{l}", [C, 1], fp32, kind="ExternalInput")
            for l in range(2)]
    newT_d = nc.dram_tensor("newT", [64, 9 * NPC], fp32,
                            kind="ExternalOutput")
    if debug:
        dbg_xg = nc.dram_tensor("dbg_xg", [128, BP, GCOLP], fp32,
                                kind="ExternalOutput")
        dbg_w = nc.dram_tensor("dbg_w", [128, BP, W3COL], fp32,
                               kind="ExternalOutput")
        dbg_msgs = nc.dram_tensor("dbg_msgs", [128, BP, F], fp32,
                                  kind="ExternalOutput")
        dbg_agg = nc.dram_tensor("dbg_agg", [128, NW, 640], fp32,
                                 kind="ExternalOutput")

    with tile.TileContext(nc) as tc, ExitStack() as ctx:
        consts = ctx.enter_context(tc.tile_pool(name="consts", bufs=1))
        idx_sb = consts.tile([128, E_PAD // 16], mybir.dt.int16)
        nc.sync.dma_start(idx_sb[:], sidx[:])
        w1_sb = consts.tile([8, H], bf16)
        nc.sync.dma_start(w1_sb[:], w1_d[:])
        b1_sb = consts.tile([H, 1], fp32)
        nc.sync.dma_start(b1_sb[:], b1_d[:])
        w2_sb = consts.tile([H, H], bf16)
        nc.sync.dma_start(w2_sb[:], w2_d[:])
        b2_sb = consts.tile([H, 1], fp32)
        nc.sync.dma_start(b2_sb[:], b2_d[:])
        w3_sb = consts.tile([H, W3COL], bf16)
        nc.sync.dma_start(w3_sb[:], w3_d[:])
        lin_sb = [consts.tile([C, C], fp32, name=f"lin{l}", tag=f"lin{l}")
                  for l in range(3)]
        for l in range(3):
            nc.sync.dma_start(lin_sb[l][:], lin_d[l][:])
        gw_sb = [consts.tile([C, C], fp32, name=f"gw{l}", tag=f"gw{l}")
                 for l in range(2)]
        gb_sb = [consts.tile([C, 1], fp32, name=f"gb{l}", tag=f"gb{l}")
                 for l in range(2)]
        for l in range(2):
            nc.sync.dma_start(gw_sb[l][:], gw_d[l][:])
            nc.sync.dma_start(gb_sb[l][:], gb_d[l][:])
        agg_sb = consts.tile([64, NW, 2, 5, 128], fp32)

        with ExitStack() as psctx:
            iop = psctx.enter_context(
                tc.tile_pool(name="iop", bufs=2 if layer2 else 3))
            aggt = psctx.enter_context(tc.tile_pool(name="aggt", bufs=1))
            wp = psctx.enter_context(tc.tile_pool(name="wp", bufs=1))
            msgp = psctx.enter_context(tc.tile_pool(name="msgp", bufs=2))
            shbp = psctx.enter_context(tc.tile_pool(name="shbp", bufs=1))
            pp = psctx.enter_context(tc.tile_pool(name="pp", bufs=1))
            zjp = psctx.enter_context(tc.tile_pool(name="zjp", bufs=1))
            zgp = psctx.enter_context(tc.tile_pool(name="zgp", bufs=1))
            h_ps = psctx.enter_context(
                tc.tile_pool(name="h_ps", bufs=2, space="PSUM"))
            w_ps_pool = psctx.enter_context(
                tc.tile_pool(name="w_ps", bufs=2, space="PSUM"))
            agg_pool = psctx.enter_context(
                tc.tile_pool(name="agg_ps", bufs=2, space="PSUM"))

            agg_open = {}

            for g in range(NG):
                t0 = g * BP
                xg = iop.tile([128, BP, GCOLP], bf16, tag="xg")
                nc.gpsimd.dma_gather(
                    out_ap=xg[:],
                    in_ap=ftab[:],
                    idxs_ap=idx_sb[:, g * (BP * 8):(g + 1) * (BP * 8)],
                    num_idxs=BP * 128,
                    num_idxs_reg=BP * 128,
                    elem_size=GCOLP,
                )
                shp_t = iop.tile([128, BP, 9], bf16, tag="shp")
                nc.sync.dma_start(shp_t[:], shp_d[:, t0:t0 + BP, :])
                if layer2:
                    shq_t = iop.tile([128, BP, NSHQ], fp32, tag="shq")
                    nc.sync.dma_start(shq_t[:], shq_d[:, t0:t0 + BP, :])
                smt = iop.tile([128, BP, 128], bf16, tag="smt")
                nc.sync.dma_start(smt[:], smat_d[:, t0:t0 + BP, :])
                bas = iop.tile([8, BP * 128], bf16, tag="bas")
                nc.sync.dma_start(bas[:], basT_d[g, :, :])

                if STAGE < 2:
                    continue
                if not layer2:
                    # sh broadcast table for L1 (ScalarE, stride-0 copy)
                    shB = shbp.tile([128, BP, 9, C], bf16, tag="shB")
                    nc.scalar.copy(
                        shB[:],
                        shp_t[:, :, :, None].broadcast_to((128, BP, 9, C)))

                # ---- radial MLP (transposed; fused Silu) ----
                h1s = iop.tile([H, BP * 128], bf16, tag="h1s")
                h2s = iop.tile([H, BP * 128], bf16, tag="h2s")
                sg = iop.tile([H, BP * 128], bf16, tag="sg")
                for c0 in range(0, BP * 128, 512):
                    h1p = h_ps.tile([H, 512], fp32, tag="h")
                    nc.tensor.matmul(h1p[:], w1_sb[:],
                                     bas[:, c0:c0 + 512], start=True, stop=True)
                    nc.scalar.activation(sg[:, c0:c0 + 512], h1p[:],
                                         AF.Sigmoid, bias=b1_sb[:, 0:1])
                    nc.vector.scalar_tensor_tensor(
                        out=h1s[:, c0:c0 + 512], in0=h1p[:],
                        scalar=b1_sb[:, 0:1], in1=sg[:, c0:c0 + 512],
                        op0=ALU.add, op1=ALU.mult)
                for c0 in range(0, BP * 128, 512):
                    h2p = h_ps.tile([H, 512], fp32, tag="h")
                    nc.tensor.matmul(h2p[:], w2_sb[:],
                                     h1s[:, c0:c0 + 512], start=True, stop=True)
                    nc.scalar.activation(sg[:, c0:c0 + 512], h2p[:],
                                         AF.Sigmoid, bias=b2_sb[:, 0:1])
                    nc.vector.scalar_tensor_tensor(
                        out=h2s[:, c0:c0 + 512], in0=h2p[:],
                        scalar=b2_sb[:, 0:1], in1=sg[:, c0:c0 + 512],
                        op0=ALU.add, op1=ALU.mult)

                # ---- per-tile edge weights w = h2s_t.T @ W3 (bf16 out) ----
                # one 1-bank PSUM tile per 512-col chunk, double-buffered:
                # tile t+1's matmul overlaps tile t's PSUM->SBUF copy
                w_sb = wp.tile([128, BP, W3COL], bf16, tag="wsb")
                for bt in range(BP):
                    for c0 in range(0, W3COL, 512):
                        c1 = min(c0 + 512, W3COL)
                        w_ps = w_ps_pool.tile([128, 512], fp32, tag="wps")
                        nc.tensor.matmul(w_ps[:, 0:c1 - c0],
                                         h2s[:, bt * 128:(bt + 1) * 128],
                                         w3_sb[:, c0:c1], start=True, stop=True)
                        nc.scalar.copy(w_sb[:, bt, c0:c1], w_ps[:, 0:c1 - c0])

                if STAGE < 3:
                    continue
                # ---- tensor product ----
                msgs = msgp.tile([128, BP, F], bf16, tag="msgs")
                if layer2:
                    _emit_tp_l2(nc, ALU, xg, w_sb, shq_t, msgs, pp, zjp, zgp)
                else:
                    _emit_tp_l1(nc, ALU, xg, w_sb, shB, msgs, pp)

                if debug and g == 0:
                    dxg = pp.tile([128, BP, GCOLP], fp32, tag="dxg")
                    nc.vector.tensor_copy(out=dxg[:], in_=xg[:])
                    nc.sync.dma_start(dbg_xg[:], dxg[:])
                    dw = pp.tile([128, BP, W3COL], fp32, tag="dw")
                    nc.vector.tensor_copy(out=dw[:], in_=w_sb[:])
                    nc.sync.dma_start(dbg_w[:], dw[:])
                    dmg = pp.tile([128, BP, F], fp32, tag="dmg")
                    nc.vector.tensor_copy(out=dmg[:], in_=msgs[:])
                    nc.sync.dma_start(dbg_msgs[:], dmg[:])

                if STAGE < 4:
                    continue
                # ---- segment sum: paired-kg bf16 matmuls, PSUM-accumulated ----
                for bt in range(BP):
                    t = t0 + bt
                    w, t_in_w = t // tpw, t % tpw
                    if w not in agg_open:
                        agg_open[w] = agg_pool.tile([128, 640], fp32,
                                                    name="aggps", tag="aggps")
                    ps = agg_open[w]
                    first, last = t_in_w == 0, t_in_w == tpw - 1
                    for pair in range(5):
                        c0 = pair * 128
                        cw = 128 if pair < 4 else 64
                        nc.tensor.matmul(
                            ps[0:cw, c0:c0 + 128],
                            msgs[:, bt, c0:c0 + cw],
                            smt[:, bt, :],
                            start=first and pair in (0, 4),
                            stop=last, skip_group_check=True)
                    if last:
                        aps = agg_open.pop(w)
                        afp = aggt.tile([128, 640], fp32, tag="afp")
                        nc.scalar.copy(afp[:, 0:512], aps[:, 0:512])
                        nc.scalar.copy(afp[0:64, 512:640], aps[0:64, 512:640])
                        nc.sync.dma_start(
                            agg_sb[:, w, 0, :, :],
                            afp[0:64, :].rearrange("p (q n) -> p q n", q=5))
                        nc.sync.dma_start(
                            agg_sb[:, w, 1, 0:4, :],
                            afp[64:128, 0:512].rearrange("p (q n) -> p q n",
                                                         q=4))



        # ---------------- per-window node update ----------------
        if STAGE < 5:
            with ExitStack() as upctx:
                upt = upctx.enter_context(tc.tile_pool(name="upt", bufs=2))
                for w in range(NW):
                    tmp = upt.tile([64, 9, 128], fp32, tag="pass")
                    nc.sync.dma_start(
                        tmp[:], oldT_d[:, :].rearrange(
                            "p (q n) -> p q n",
                            q=9)[:, :, w * 128:(w + 1) * 128])
                    nc.sync.dma_start(
                        newT_d[:, :].rearrange(
                            "p (q n) -> p q n",
                            q=9)[:, :, w * 128:(w + 1) * 128],
                        tmp[:])
            nc.compile()
            return nc
        with ExitStack() as upctx:
            y_pool = upctx.enter_context(
                tc.tile_pool(name="y_ps", bufs=2, space="PSUM"))
            g_pool = upctx.enter_context(
                tc.tile_pool(name="g_ps", bufs=2, space="PSUM"))
            upt = upctx.enter_context(tc.tile_pool(name="upt", bufs=2))
            for w in range(NW):
                oldw = upt.tile([64, 9, 128], fp32, tag="oldw")
                nc.sync.dma_start(
                    oldw[:], oldT_d[:, :].rearrange(
                        "p (q n) -> p q n", q=9)[:, :, w * 128:(w + 1) * 128])
                neww = upt.tile([64, 9, 128], fp32, tag="neww")
                y_ps = y_pool.tile([64, 9 * 128], fp32, tag="yps")
                for kg in range(9):
                    l = 0 if kg == 0 else (1 if kg <= 3 else 2)
                    nc.tensor.matmul(
                        y_ps[:, kg * 128:(kg + 1) * 128],
                        lin_sb[l][:],
                        agg_sb[:, w, kg % 2, kg // 2, :],
                        start=kg in (0, 4, 8), stop=True,
                        skip_group_check=True)
                y0g = upt.tile([C, 128], fp32, tag="y0g")
                nc.scalar.activation(y0g[:], y_ps[:, 0:128], AF.Sigmoid)
                y0s = upt.tile([C, 128], fp32, tag="y0s")
                nc.vector.tensor_tensor(out=y0s[:], in0=y_ps[:, 0:128],
                                        in1=y0g[:], op=ALU.mult)
                nc.vector.tensor_tensor(out=neww[:, 0, :], in0=y0s[:],
                                        in1=oldw[:, 0, :], op=ALU.add)
                g_ps = g_pool.tile([C, 2, 128], fp32, tag="gps")
                for l in (1, 2):
                    nc.tensor.matmul(g_ps[:, l - 1, :], gw_sb[l - 1][:],
                                     neww[:, 0, :], start=(l == 1), stop=True,
                                     skip_group_check=True)
                gts = upt.tile([C, 2, 128], fp32, tag="gts")
                for l in (1, 2):
                    nc.scalar.activation(gts[:, l - 1, :], g_ps[:, l - 1, :],
                                         AF.Sigmoid, bias=gb_sb[l - 1][:, 0:1])
                gy = upt.tile([C, 8, 128], fp32, tag="gy")
                nc.vector.tensor_tensor(
                    out=gy[:, 0:3, :],
                    in0=y_ps[:].rearrange("p (q n) -> p q n", q=9)[:, 1:4, :],
                    in1=gts[:, 0:1, :].broadcast_to((C, 3, 128)),
                    op=ALU.mult)
                nc.vector.tensor_tensor(
                    out=gy[:, 3:8, :],
                    in0=y_ps[:].rearrange("p (q n) -> p q n", q=9)[:, 4:9, :],
                    in1=gts[:, 1:2, :].broadcast_to((C, 5, 128)),
                    op=ALU.mult)
                nc.vector.tensor_tensor(out=neww[:, 1:9, :], in0=gy[:],
                                        in1=oldw[:, 1:9, :], op=ALU.add)
                nc.sync.dma_start(
                    newT_d[:, :].rearrange("p (q n) -> p q n",
                                           q=9)[:, :, w * 128:(w + 1) * 128],
                    neww[:])

    nc.compile()
    return nc


def _emit_tp_l2(nc, ALU, xg, w_sb, shq_t, msgs, pp, zjp, zgp):
    """P products, cg-folded zjQ scalings, FMA as adds (+8 ratio stts).

    All on DVE: single in-order queue, no cross-engine ping-pong; GpSimd
    has ~9us fixed cost per elementwise op and ScalarE cannot multiply
    two tensors, so the TP bulk lives here.
    """
    import concourse.mybir as mybir
    bf16 = mybir.dt.bfloat16
    fp32 = mybir.dt.float32

    GP_JS = ()          # GpSimd streams ~0.4 elem/cyc and zjG bufs=1
                        # serialized groups -> ~19us DVE stall per group;
                        # cheaper to keep all zjq on DVE (nnz-only now)
    EXPC = 8            # ping-pong: 2 x 8x8x32 fp32 = 16KB
    P0 = pp.tile([128, BP, SCHED2_L2[0]["nblk"] * C], bf16, name="P0", tag="P0")
    P12 = pp.tile([128, BP, SCHED2_L2[2]["nblk"] * C], bf16,
                  name="P12", tag="P12")
    zj12 = zjp.tile([128, BP, SCHED2_L2[2]["nblk"] * C], bf16,
                    name="zj12", tag="zj12")
    exps = [zgp.tile([128, BP, EXPC, C // 2], fp32, name=f"exp{i}",
                     tag=f"exp{i}") for i in range(2)]
    ppg = [0]

    def emit_zjq(eng, dst, P, qoff, segs):
        # shq host-packed as fp32 = (v,v) bf16 pair: ScalarE broadcast-
        # expands at half the element count, DVE bitcasts back to bf16 and
        # multiplies with unit-stride in1 -> 2x_1P mode. 4-deep ping-pong
        # so expansion runs ahead of consumption across FMA phases.
        chunks, cur, used = [], [], 0
        for (z0, L) in segs:
            while L > 0:
                if used == EXPC:
                    chunks.append(cur)
                    cur, used = [], 0
                take = min(L, EXPC - used)
                cur.append((z0, take, used))
                z0, L, used = z0 + take, L - take, used + take
        if cur:
            chunks.append(cur)
        for ch in chunks:
            exp = exps[ppg[0]]
            ppg[0] = (ppg[0] + 1) % 2
            for (z0, L, eo) in ch:
                nc.scalar.copy(
                    exp[:, :, eo:eo + L, :],
                    shq_t[:, :, qoff + z0:qoff + z0 + L, None].broadcast_to(
                        (128, BP, L, C // 2)))
            expb = exp[:].bitcast(mybir.dt.bfloat16)
            for (z0, L, eo) in ch:
                eng.tensor_tensor(
                    out=dst[:].rearrange("p b (n c) -> p b n c",
                                         c=C)[:, :, z0:z0 + L, :],
                    in0=P[:].rearrange("p b (n c) -> p b n c",
                                       c=C)[:, :, z0:z0 + L, :],
                    in1=expb[:, :, eo:eo + L, :],
                    op=ALU.mult)

    def emit_fma(sch, j, zj):
        for (jj, z0, k0, L) in sch["runs_add"]:
            if jj != j:
                continue
            nc.vector.tensor_tensor(
                out=msgs[:, :, k0 * C:(k0 + L) * C],
                in0=zj[:, :, z0 * C:(z0 + L) * C],
                in1=msgs[:, :, k0 * C:(k0 + L) * C],
                op=ALU.add)
        for (jj, z, k, ratio) in sch["runs_stt"]:
            if jj != j:
                continue
            nc.vector.scalar_tensor_tensor(
                out=msgs[:, :, k * C:(k + 1) * C],
                in0=zj[:, :, z * C:(z + 1) * C],
                scalar=float(ratio),
                in1=msgs[:, :, k * C:(k + 1) * C],
                op0=ALU.mult, op1=ALU.add)

    for gi, sch in enumerate(SCHED2_L2):
        nblk = sch["nblk"]
        P = P0 if gi == 0 else P12
        for (p, ig0, ni, z0) in sch["pops"]:
            nc.vector.tensor_tensor(
                out=P[:].rearrange("p b (n c) -> p b n c",
                                   c=C)[:, :, z0:z0 + ni, :],
                in0=xg[:].rearrange("p b (n c) -> p b n c",
                                    c=C)[:, :, ig0:ig0 + ni, :],
                in1=w_sb[:, :, None, p * C:(p + 1) * C].broadcast_to(
                    (128, BP, ni, C)),
                op=ALU.mult)
        if gi > 0:
            # kick GpSimd's share first so it overlaps DVE's other j's
            for (j, qoff, segs) in sch["zjq"]:
                if j in GP_JS:
                    emit_zjq(nc.gpsimd, zjG[j], P, qoff, segs)
        for (j, qoff, segs) in sch["zjq"]:
            if gi > 0 and j in GP_JS:
                continue
            dst = msgs if gi == 0 else zj12
            emit_zjq(nc.vector, dst, P, qoff, segs)
            if gi == 0:
                continue
            emit_fma(sch, j, zj12)
        if gi > 0:
            for j in GP_JS:
                if any(jj == j for (jj, _, _) in sch["zjq"]):
                    emit_fma(sch, j, zjG[j])


def _emit_tp_l1(nc, ALU, xg, w_sb, shB, msgs, pp):
    """msgs[k] = x * w'_k * sh_k ; w' host-expanded+cg-folded to 9 kg blocks."""
    import concourse.mybir as mybir
    bf16 = mybir.dt.bfloat16
    tmp = pp.tile([128, BP, 9, C], bf16, name="l1tmp", tag="l1tmp")
    nc.vector.tensor_tensor(
        out=tmp[:],
        in0=w_sb[:].rearrange("p b (n c) -> p b n c", c=C),
        in1=shB[:],
        op=ALU.mult)
    nc.vector.tensor_tensor(
        out=msgs[:].rearrange("p b (n c) -> p b n c", c=C),
        in0=tmp[:],
        in1=xg[:, :, None, 0:C].broadcast_to((128, BP, 9, C)),
        op=ALU.mult)


# ---------------- host orchestration ----------------
def _chunked_T(feats_own):
    """[NPC, 576] -> kg-blocked transposed [64, 9*NPC]."""
    out = np.empty((64, 9 * NPC), np.float32)
    for kg in range(9):
        out[:, kg * NPC:(kg + 1) * NPC] = feats_own[:, kg * 64:(kg + 1) * 64].T
    return out


def _unchunk_T(newT):
    """[64, 9*NPC] -> [NPC, 576]."""
    out = np.empty((NPC, 576), np.float32)
    for kg in range(9):
        out[:, kg * 64:(kg + 1) * 64] = newT[:, kg * NPC:(kg + 1) * NPC].T
    return out


_CACHE = {}


def _prep(positions, senders, receivers):
    key = (senders.tobytes(), receivers.tobytes(), positions.tobytes())
    if _CACHE.get("key") == key:
        return _CACHE["val"]
    sh_eff, basis = edge_geometry(positions, senders, receivers)
    owner, local, nodes_of, _ = partition_graph(receivers)
    deg_bin = np.zeros(NCORES * NW, np.int64)
    np.add.at(deg_bin, owner[receivers] * NW + local[receivers] // 128, 1)
    tpw = (int(deg_bin.max()) + 127) // 128
    T = NW * tpw
    assert T % BP == 0
    perm = build_core_edges(receivers, owner, local, tpw)

    valid = perm >= 0
    eg = np.where(valid, perm, 0)
    snd = np.where(valid, senders[eg], 0).astype(np.int16)      # [NC, T*128]
    shp_e = sh_eff[eg] * valid[..., None]                        # [NC, T*128, 9]
    bas_e = basis[eg] * valid[..., None]                         # [NC, T*128, 8]
    lr = np.where(valid, local[receivers[eg]], 0)

    NG = T // BP
    inv = np.float32(1.0 / np.sqrt(AVG_NN))
    sidx = np.empty((NCORES, 128, T * 128 // 16), np.int16)
    shp_h = np.empty((NCORES, 128, T, 9), BF16)
    shq_h = np.empty((NCORES, 128, T, NSHQ), np.float32)
    jcols = np.array([j for (j, cg) in SHQ_COLS])
    cgv = np.array([cg for (j, cg) in SHQ_COLS], np.float32)
    bas_h = np.empty((NCORES, NG, 8, BP * 128), BF16)
    smat_h = np.zeros((NCORES, 128, T, 128), BF16)
    for k in range(NCORES):
        s = snd[k].reshape(T * 8, 16)
        sidx[k] = np.tile(s.T, (8, 1))
        shp_f = shp_e[k].reshape(T, 128, 9).transpose(1, 0, 2)
        shp_h[k] = shp_f.astype(BF16)
        # fp32-packed (v, v) bf16 pair so on-chip broadcast-expansion moves
        # half the elements (bitcast back to bf16 at the consumer)
        u = (shp_f[:, :, jcols] * cgv[None, None, :]).astype(
            BF16).view(np.uint16).astype(np.uint32)
        shq_h[k] = ((u << 16) | u).view(np.float32)
        bas_h[k] = bas_e[k].reshape(NG, BP * 128, 8).transpose(0, 2, 1).astype(BF16)
        v = valid[k]
        e_slots = np.arange(T * 128)
        p_, t_ = e_slots % 128, e_slots // 128
        cols = lr[k] - (t_ // tpw) * 128
        ok = v & (cols >= 0) & (cols < 128)
        sm = np.zeros((128, T, 128), np.float32)
        sm[p_[ok], t_[ok], cols[ok]] = inv
        smat_h[k] = sm.astype(BF16)
    val = dict(T=T, NG=NG, tpw=tpw, nodes_of=nodes_of, sidx=sidx,
               shp_h=shp_h, bas_h=bas_h, smat_h=smat_h, shq_h=shq_h)
    _CACHE["key"], _CACHE["val"] = key, val
    return val


EXEC_NS = []


def _run_layer(nc, pre, ftab, oldT_by_core, lw):
    import os
    from concourse.bass_utils import run_bass_kernel_spmd
    in_maps = []
    for k in range(NCORES):
        m = dict(ftab=ftab,
                 sidx=pre["sidx"][k],
                 shp=pre["shp_h"][k],
                 shq=pre["shq_h"][k],
                 basisT=pre["bas_h"][k],
                 smat=pre["smat_h"][k],
                 oldT=oldT_by_core[k],
                 w1=lw["w1"], b1=lw["b1"], w2=lw["w2"], b2=lw["b2"],
                 w3=lw["w3"], lin0=lw["lin"][0], lin1=lw["lin"][1],
                 lin2=lw["lin"][2], gw0=lw["gw"][0], gw1=lw["gw"][1],
                 gb0=lw["gb"][0], gb1=lw["gb"][1])
        in_maps.append(m)
    trace = bool(os.environ.get("KERNEL_TRACE"))
    res = run_bass_kernel_spmd(nc, in_maps, list(range(NCORES)), trace=trace,
                               trace_cores=list(range(NCORES)) if trace else None)
    if trace and res.exec_time_ns is not None:
        EXEC_NS.append(res.exec_time_ns)
    return [res.results[k]["newT"] for k in range(NCORES)]


def _layer_weights(inputs, i, layer2):
    f32 = np.float32
    w3 = np.ascontiguousarray(inputs["mlp_w3"][i], f32)
    if layer2:
        w3p = w3.astype(BF16)
    else:
        w3p = np.empty((H, 576), BF16)
        for kg in range(9):
            p = L1_PATH_OF_K[kg]
            w3p[:, kg * C:(kg + 1) * C] = (
                w3[:, p * C:(p + 1) * C] * L1_CG_OF_K[kg]).astype(BF16)
    return dict(
        w1=np.ascontiguousarray(inputs["mlp_w1"][i], f32).astype(BF16),
        b1=np.ascontiguousarray(inputs["mlp_b1"][i], f32).reshape(H, 1),
        w2=np.ascontiguousarray(inputs["mlp_w2"][i], f32).astype(BF16),
        b2=np.ascontiguousarray(inputs["mlp_b2"][i], f32).reshape(H, 1),
        w3=w3p,
        lin=[np.ascontiguousarray(inputs["lin_self"][i, l], f32)
             for l in range(3)],
        gw=[np.ascontiguousarray(inputs["gate_w"][i, l], f32)
            for l in range(2)],
        gb=[np.ascontiguousarray(inputs["gate_b"][i, l], f32).reshape(C, 1)
            for l in range(2)],
    )


_KERNEL_CACHE = {}


def _get_kernels(T):
    if T not in _KERNEL_CACHE:
        _KERNEL_CACHE[T] = (build_layer_kernel(False, T),
                            build_layer_kernel(True, T))
    return _KERNEL_CACHE[T]


def _pack_ftab(table, ncols):
    out = np.zeros((N_NODES, ncols), BF16)
    used = min(ncols, table.shape[1])
    out[:, :used] = table[:, :used].astype(BF16)
    return out


def kernel(**inputs):
    positions = np.asarray(inputs["positions"], np.float32)
    species = np.asarray(inputs["species"]).astype(np.int64)
    senders = np.asarray(inputs["senders"]).astype(np.int64)
    receivers = np.asarray(inputs["receivers"]).astype(np.int64)

    pre = _prep(positions, senders, receivers)
    T = pre["T"]
    nc1, nc2 = _get_kernels(T)
    nodes_of = pre["nodes_of"]

    # initial features: x0 from species embedding (host; tiny)
    x0 = (np.asarray(inputs["embed"], np.float32)[species]
          @ np.asarray(inputs["w_proj"], np.float32))          # [N, 64]
    table = np.zeros((N_NODES, F), np.float32)
    table[:, 0:C] = x0

    # ---- layer 1 ----
    oldT = [_chunked_T(table[nodes_of[k]]) for k in range(NCORES)]
    lw = _layer_weights(inputs, 0, False)
    newT = _run_layer(nc1, pre, _pack_ftab(table, GCOLP_L1), oldT, lw)

    table2 = np.empty((N_NODES, F), np.float32)
    for k in range(NCORES):
        table2[nodes_of[k]] = _unchunk_T(newT[k])

    # ---- layer 2 ----
    lw = _layer_weights(inputs, 1, True)
    newT2 = _run_layer(nc2, pre, _pack_ftab(table2, GCOLP_L2), newT, lw)

    table3 = np.empty((N_NODES, F), np.float32)
    for k in range(NCORES):
        table3[nodes_of[k]] = _unchunk_T(newT2[k])

    # ---- output: reorder component-major -> reference layout + alpha ----
    t3 = table3.reshape(N_NODES, 9, C)
    out = np.empty((N_NODES, F), np.float32)
    out[:, 0:64] = t3[:, 0]
    out[:, 64:256] = (0.5 * t3[:, 1:4]).transpose(0, 2, 1).reshape(N_NODES, 192)
    out[:, 256:576] = (0.25 * t3[:, 4:9]).transpose(0, 2, 1).reshape(N_NODES, 320)
    return out



# revision 25
# speedup vs baseline: 1.3541x; 1.0021x over previous
"""NequIP GNN message-passing kernel for 8 Trainium2 NeuronCores — v2.

Receiver-sharded graph parallelism (per sharding hint): host LPT-assigns the
8192 nodes to 64 (core, window) bins of 128, each core owns 8 windows = 1024
nodes + their in-edges, sorted by window, padded to 128-edge tiles.

v2 device pipeline (vs v1): bf16 edge pipeline end-to-end with fp32 PSUM
accumulation; per-edge-scalar stages batched across 8-tile groups via
stride-0 broadcast access patterns; P-products collapsed per-path (w operand
broadcast over l1 components); CG-FMA stage merged into diagonal runs with
immediate scalars; segment-sum via paired-kg bf16 matmuls; radial MLP with
fused Silu activations; elementwise work split across Vector/GpSimd/Scalar.
"""
import math
import numpy as np
import ml_dtypes

BF16 = ml_dtypes.bfloat16

# ---------------- model constants ----------------
N_NODES, N_EDGES = 8192, 131072
C, H, NRAD = 64, 64, 8
R_MAX, AVG_NN = 5.0, 16.0
NCORES, NPC = 8, 1024
NW = NPC // 128
F = 9 * C
LS = (0, 1, 2)
PATHS = [(l1, l2, l3) for l1 in LS for l2 in LS for l3 in LS
         if abs(l1 - l2) <= l3 <= l1 + l2]
LOFF = {0: 0, 1: 1, 2: 4}
J_OF_L2 = {0: [0], 1: [1, 2, 3], 2: [4, 5, 6, 7, 8]}
BP = 8                      # tiles per group (batched in op free dims)
GCOLP_L2 = 640              # padded gather row (bf16): 1280B % 256 == 0
GCOLP_L1 = 128              # 256B % 256 == 0


# ---------------- real Clebsch-Gordan coefficients ----------------
def _cg_scalar(j1, m1, j2, m2, j3, m3):
    f = math.factorial
    if m1 + m2 != m3:
        return 0.0
    pre = ((2*j3+1) * f(j1+j2-j3) * f(j1-j2+j3) * f(-j1+j2+j3)
           / f(j1+j2+j3+1)) ** 0.5
    pre *= (f(j1+m1)*f(j1-m1)*f(j2+m2)*f(j2-m2)*f(j3+m3)*f(j3-m3)) ** 0.5
    s = 0.0
    for k in range(max(0, j2-j3-m1, j1+m2-j3), min(j1+j2-j3, j1-m1, j2+m2)+1):
        s += (-1)**k / (f(k)*f(j1+j2-j3-k)*f(j1-m1-k)
                        * f(j2+m2-k)*f(j3-j2+m1+k)*f(j3-j1-m2+k))
    return pre * s


def _U_real(l):
    U = np.zeros((2*l+1, 2*l+1), dtype=complex)
    s2 = 2 ** -0.5
    for m in range(-l, l+1):
        if m > 0:
            U[m+l, m+l] = (-1)**m * s2
            U[m+l, -m+l] = s2
        elif m == 0:
            U[l, l] = 1.0
        else:
            U[m+l, m+l] = 1j*s2
            U[m+l, -m+l] = -1j*(-1)**(-m)*s2
    return U


def _real_cg(l1, l2, l3):
    Cc = np.zeros((2*l1+1, 2*l2+1, 2*l3+1))
    for i1, m1 in enumerate(range(-l1, l1+1)):
        for i2, m2 in enumerate(range(-l2, l2+1)):
            m3 = m1 + m2
            if abs(m3) <= l3:
                Cc[i1, i2, m3+l3] = _cg_scalar(l1, m1, l2, m2, l3, m3)
    U1, U2, U3 = _U_real(l1), _U_real(l2), _U_real(l3)
    W = np.einsum('ia,jb,kc,abc->ijk', U1.conj(), U2.conj(), U3,
                  Cc.astype(complex))
    W = W.real if np.linalg.norm(W.real) >= np.linalg.norm(W.imag) else W.imag
    W = W / np.linalg.norm(W) * (2*l3+1) ** 0.5
    return np.asarray(W, dtype=np.float64)


CGS = [_real_cg(*p) for p in PATHS]


def build_schedule_l2():
    """Static TP structure for layer 2, grouped by l2.

    Per group: nblk, pops=[(path, ig0, ni, z0)], jlist, runs=[(j,z0,k0,L,cg)].
    """
    scheds = []
    for l2 in (0, 1, 2):
        ps = [p for p in range(len(PATHS)) if PATHS[p][1] == l2]
        blocks, block_of, pops = [], {}, []
        for p in ps:
            l1 = PATHS[p][0]
            ni = 2 * l1 + 1
            pops.append((p, LOFF[l1], ni, len(blocks)))
            for i in range(ni):
                block_of[(p, i)] = len(blocks)
                blocks.append((p, i))
        tset = set()
        for p in ps:
            l1, _, l3 = PATHS[p]
            cg = CGS[p]
            for i in range(2*l1+1):
                for j in range(2*l2+1):
                    for k in range(2*l3+1):
                        v = cg[i, j, k]
                        if abs(v) > 1e-12:
                            tset.add((LOFF[l2]+j, block_of[(p, i)],
                                      LOFF[l3]+k, round(float(v), 9)))
        runs, consumed = [], set()
        for t in sorted(tset):
            if t in consumed:
                continue
            j, z, k, cgv = t
            L = 0
            while (j, z+L, k+L, cgv) in tset and (j, z+L, k+L, cgv) not in consumed:
                consumed.add((j, z+L, k+L, cgv))
                L += 1
            runs.append((j, z, k, L, cgv))
        scheds.append(dict(l2=l2, nblk=len(blocks), pops=pops,
                           jlist=J_OF_L2[l2], runs=runs))
    return scheds


SCHED_L2 = build_schedule_l2()


def build_schedule2():
    """cg folded into per-(j,z) host scalars; FMA mostly tensor adds.

    Per group: nblk, pops, zjq=[(j, qoff)], runs_add=[(j,z0,k0,L)],
    runs_stt=[(j,z,k,ratio)]. qoff = column offset of (j,*) block in the
    concatenated shq table [128, T, 179]. cgfold[qoff+z] = cg of the primary
    (smallest-k) triple of (j,z); 0 for unused (z,j).
    """
    scheds, qoff, cgcols = [], 0, []
    for gi, sch in enumerate(SCHED_L2):
        nblk = sch["nblk"]
        tset = set()
        for (j, z, k, L, cg) in sch["runs"]:
            for i in range(L):
                tset.add((j, z + i, k + i, cg))
        per_jz = {}
        for (j, z, k, cg) in tset:
            per_jz.setdefault((j, z), []).append((k, cg))
        primary, runs_stt = set(), []
        cgf = {}
        for (j, z), ks in per_jz.items():
            ks.sort()
            k0, cg0 = ks[0]
            primary.add((j, z, k0))
            cgf[(j, z)] = cg0
            for (k1, cg1) in ks[1:]:
                runs_stt.append((j, z, k1, cg1 / cg0))
        runs_add, consumed = [], set()
        for t in sorted(primary):
            if t in consumed:
                continue
            j, z, k = t
            L = 0
            while (j, z + L, k + L) in primary and (j, z + L, k + L) not in consumed:
                consumed.add((j, z + L, k + L))
                L += 1
            runs_add.append((j, z, k, L))
        zjq = []
        for j in sch["jlist"]:
            # contiguous segments of z columns actually used by (j, z):
            # skips the ~28% of dense columns with cg == 0
            used = sorted(z for (jj, z) in cgf if jj == j)
            segs, s0, prev = [], None, None
            for z in used:
                if s0 is None:
                    s0 = prev = z
                elif z == prev + 1:
                    prev = z
                else:
                    segs.append((s0, prev - s0 + 1))
                    s0 = prev = z
            if s0 is not None:
                segs.append((s0, prev - s0 + 1))
            zjq.append((j, qoff, segs))
            for z in range(nblk):
                cgcols.append((j, cgf.get((j, z), 0.0)))
            qoff += nblk
        scheds.append(dict(nblk=nblk, pops=sch["pops"], zjq=zjq,
                           runs_add=runs_add, runs_stt=runs_stt))
    return scheds, cgcols


SCHED2_L2, SHQ_COLS = build_schedule2()
NSHQ = len(SHQ_COLS)

# layer-1 per-k path and cg (x is scalar-only: paths (0,l,l), j == k)
L1_PATH_OF_K = [0] + [1]*3 + [2]*5
L1_CG_OF_K = []
for _k in range(9):
    _p = L1_PATH_OF_K[_k]
    _l = PATHS[_p][2]
    _m = _k - LOFF[_l]
    L1_CG_OF_K.append(float(CGS[_p][0, _m, _m]))


# ---------------- host-side graph preprocessing ----------------
def edge_geometry(positions, senders, receivers):
    rel = (positions[receivers] - positions[senders]) / R_MAX
    d = np.linalg.norm(rel, axis=-1)
    u = rel / np.maximum(d, 1e-6)[:, None]
    x, y, z = u[:, 0], u[:, 1], u[:, 2]
    sh = np.empty((len(d), 9), np.float32)
    sh[:, 0] = 1.0
    sh[:, 1] = np.sqrt(3.0) * y
    sh[:, 2] = np.sqrt(3.0) * z
    sh[:, 3] = np.sqrt(3.0) * x
    sh[:, 4] = np.sqrt(15.0) * x * y
    sh[:, 5] = np.sqrt(15.0) * y * z
    sh[:, 6] = np.sqrt(5.0) / 2 * (3 * z * z - 1.0)
    sh[:, 7] = np.sqrt(15.0) * x * z
    sh[:, 8] = np.sqrt(15.0) / 2 * (x * x - y * y)
    freqs = np.arange(1, NRAD + 1, dtype=np.float64)
    xr = np.clip(d, 1e-4, 1.0)[:, None].astype(np.float64)
    basis = (np.sqrt(2.0) * np.sin(freqs * np.pi * xr) / xr).astype(np.float32)
    cut = (0.5 * (np.cos(np.pi * np.clip(d, 0.0, 1.0)) + 1.0)).astype(np.float32)
    return (sh * cut[:, None]).astype(np.float32), basis


def partition_graph(receivers):
    import heapq
    deg = np.bincount(receivers, minlength=N_NODES)
    order = np.argsort(-deg, kind="stable")
    nbins = NCORES * NW
    load = np.zeros(nbins, np.int64)
    cnt = np.zeros(nbins, np.int64)
    owner = np.empty(N_NODES, np.int32)
    local = np.empty(N_NODES, np.int32)
    heap = [(0, b) for b in range(nbins)]
    heapq.heapify(heap)
    for n in order:
        while True:
            l, b = heapq.heappop(heap)
            if cnt[b] < 128:
                break
        owner[n] = b // NW
        local[n] = (b % NW) * 128 + cnt[b]
        cnt[b] += 1
        load[b] += deg[n]
        if cnt[b] < 128:
            heapq.heappush(heap, (int(load[b]), b))
    nodes_of = np.empty((NCORES, NPC), np.int64)
    for n in range(N_NODES):
        nodes_of[owner[n], local[n]] = n
    return owner, local, nodes_of, int(load.max())


def build_core_edges(receivers, owner, local, tpw):
    T = NW * tpw
    perm = np.full((NCORES, T * 128), -1, np.int64)
    for k in range(NCORES):
        eids = np.where(owner[receivers] == k)[0]
        lr = local[receivers[eids]]
        o = np.argsort(lr, kind="stable")
        eids, lr = eids[o], lr[o]
        w_of = lr // 128
        for w in range(NW):
            sel = eids[w_of == w]
            assert len(sel) <= tpw * 128, "tiles-per-window overflow"
            base = w * tpw * 128
            perm[k, base:base + len(sel)] = sel
    return perm


# ---------------- bass kernel builder ----------------
def build_layer_kernel(layer2, T, debug=False):
    import concourse.bass as bass
    import concourse.bacc as bacc
    import concourse.tile as tile
    import concourse.mybir as mybir
    from contextlib import ExitStack

    fp32 = mybir.dt.float32
    bf16 = mybir.dt.bfloat16
    AF = mybir.ActivationFunctionType
    ALU = mybir.AluOpType

    NPATH = 15 if layer2 else 3
    GCOLP = GCOLP_L2 if layer2 else GCOLP_L1
    W3COL = NPATH * C if layer2 else 576   # L1 w3 host-expanded to 9 kg blocks
    E_PAD = T * 128
    NG = T // BP
    assert T % BP == 0 and T % NW == 0
    tpw = T // NW

    import os as _os
    STAGE = int(_os.environ.get("KV2_STAGE", "5"))
    nc = bacc.Bacc("TRN2", target_bir_lowering=False)

    ftab = nc.dram_tensor("ftab", [N_NODES, GCOLP], bf16, kind="ExternalInput")
    sidx = nc.dram_tensor("sidx", [128, E_PAD // 16], mybir.dt.int16,
                          kind="ExternalInput")
    shp_d = nc.dram_tensor("shp", [128, T, 9], fp32, kind="ExternalInput")
    shq_d = nc.dram_tensor("shq", [128, T, NSHQ], fp32, kind="ExternalInput")
    basT_d = nc.dram_tensor("basisT", [NG, 8, BP * 128], bf16,
                            kind="ExternalInput")
    smat_d = nc.dram_tensor("smat", [128, T, 128], bf16, kind="ExternalInput")
    oldT_d = nc.dram_tensor("oldT", [64, 9 * NPC], fp32, kind="ExternalInput")
    w1_d = nc.dram_tensor("w1", [8, H], bf16, kind="ExternalInput")
    b1_d = nc.dram_tensor("b1", [H, 1], fp32, kind="ExternalInput")
    w2_d = nc.dram_tensor("w2", [H, H], bf16, kind="ExternalInput")
    b2_d = nc.dram_tensor("b2", [H, 1], fp32, kind="ExternalInput")
    w3_d = nc.dram_tensor("w3", [H, W3COL], bf16, kind="ExternalInput")
    lin_d = [nc.dram_tensor(f"lin{l}", [C, C], fp32, kind="ExternalInput")
             for l in range(3)]
    gw_d = [nc.dram_tensor(f"gw{l}", [C, C], fp32, kind="ExternalInput")
            for l in range(2)]
    gb_d = [nc.dram_tensor(f"gb{l}", [C, 1], fp32, kind="ExternalInput")
            for l in range(2)]
    newT_d = nc.dram_tensor("newT", [64, 9 * NPC], fp32,
                            kind="ExternalOutput")
    if debug:
        dbg_xg = nc.dram_tensor("dbg_xg", [128, BP, GCOLP], fp32,
                                kind="ExternalOutput")
        dbg_w = nc.dram_tensor("dbg_w", [128, BP, W3COL], fp32,
                               kind="ExternalOutput")
        dbg_msgs = nc.dram_tensor("dbg_msgs", [128, BP, F], fp32,
                                  kind="ExternalOutput")
        dbg_agg = nc.dram_tensor("dbg_agg", [128, NW, 640], fp32,
                                 kind="ExternalOutput")

    with tile.TileContext(nc) as tc, ExitStack() as ctx:
        consts = ctx.enter_context(tc.tile_pool(name="consts", bufs=1))
        idx_sb = consts.tile([128, E_PAD // 16], mybir.dt.int16)
        nc.sync.dma_start(idx_sb[:], sidx[:])
        w1_sb = consts.tile([8, H], bf16)
        nc.sync.dma_start(w1_sb[:], w1_d[:])
        b1_sb = consts.tile([H, 1], fp32)
        nc.sync.dma_start(b1_sb[:], b1_d[:])
        w2_sb = consts.tile([H, H], bf16)
        nc.sync.dma_start(w2_sb[:], w2_d[:])
        b2_sb = consts.tile([H, 1], fp32)
        nc.sync.dma_start(b2_sb[:], b2_d[:])
        w3_sb = consts.tile([H, W3COL], bf16)
        nc.sync.dma_start(w3_sb[:], w3_d[:])
        lin_sb = [consts.tile([C, C], fp32, name=f"lin{l}", tag=f"lin{l}")
                  for l in range(3)]
        for l in range(3):
            nc.sync.dma_start(lin_sb[l][:], lin_d[l][:])
        gw_sb = [consts.tile([C, C], fp32, name=f"gw{l}", tag=f"gw{l}")
                 for l in range(2)]
        gb_sb = [consts.tile([C, 1], fp32, name=f"gb{l}", tag=f"gb{l}")
                 for l in range(2)]
        for l in range(2):
            nc.sync.dma_start(gw_sb[l][:], gw_d[l][:])
            nc.sync.dma_start(gb_sb[l][:], gb_d[l][:])
        agg_sb = consts.tile([64, NW, 2, 5, 128], fp32)

        with ExitStack() as psctx:
            iop = psctx.enter_context(
                tc.tile_pool(name="iop", bufs=2 if layer2 else 3))
            aggt = psctx.enter_context(tc.tile_pool(name="aggt", bufs=1))
            wp = psctx.enter_context(tc.tile_pool(name="wp", bufs=1))
            msgp = psctx.enter_context(tc.tile_pool(name="msgp", bufs=2))
            shbp = psctx.enter_context(tc.tile_pool(name="shbp", bufs=1))
            pp = psctx.enter_context(tc.tile_pool(name="pp", bufs=1))
            zjp = psctx.enter_context(tc.tile_pool(name="zjp", bufs=1))
            zgp = psctx.enter_context(tc.tile_pool(name="zgp", bufs=1))
            h_ps = psctx.enter_context(
                tc.tile_pool(name="h_ps", bufs=2, space="PSUM"))
            w_ps_pool = psctx.enter_context(
                tc.tile_pool(name="w_ps", bufs=2, space="PSUM"))
            agg_pool = psctx.enter_context(
                tc.tile_pool(name="agg_ps", bufs=2, space="PSUM"))

            agg_open = {}

            for g in range(NG):
                t0 = g * BP
                xg = iop.tile([128, BP, GCOLP], bf16, tag="xg")
                nc.gpsimd.dma_gather(
                    out_ap=xg[:],
                    in_ap=ftab[:],
                    idxs_ap=idx_sb[:, g * (BP * 8):(g + 1) * (BP * 8)],
                    num_idxs=BP * 128,
                    num_idxs_reg=BP * 128,
                    elem_size=GCOLP,
                )
                if not layer2:
                    shp_t = iop.tile([128, BP, 9], fp32, tag="shp")
                    nc.sync.dma_start(shp_t[:], shp_d[:, t0:t0 + BP, :])
                if layer2:
                    shq_t = iop.tile([128, BP, NSHQ], fp32, tag="shq")
                    nc.sync.dma_start(shq_t[:], shq_d[:, t0:t0 + BP, :])
                smt = iop.tile([128, BP, 128], bf16, tag="smt")
                nc.sync.dma_start(smt[:], smat_d[:, t0:t0 + BP, :])
                bas = iop.tile([8, BP * 128], bf16, tag="bas")
                nc.sync.dma_start(bas[:], basT_d[g, :, :])

                if STAGE < 2:
                    continue
                if not layer2:
                    # sh broadcast table for L1: fp32 = packed (v,v) bf16
                    # pair, so ScalarE moves half the elements (1x mode)
                    shB = shbp.tile([128, BP, 9, C // 2], fp32, tag="shB")
                    nc.scalar.copy(
                        shB[:],
                        shp_t[:, :, :, None].broadcast_to(
                            (128, BP, 9, C // 2)))

                # ---- radial MLP (transposed; fused Silu) ----
                h1s = iop.tile([H, BP * 128], bf16, tag="h1s")
                h2s = iop.tile([H, BP * 128], bf16, tag="h2s")
                sg = iop.tile([H, BP * 128], bf16, tag="sg")
                for c0 in range(0, BP * 128, 512):
                    h1p = h_ps.tile([H, 512], fp32, tag="h")
                    nc.tensor.matmul(h1p[:], w1_sb[:],
                                     bas[:, c0:c0 + 512], start=True, stop=True)
                    nc.scalar.activation(sg[:, c0:c0 + 512], h1p[:],
                                         AF.Sigmoid, bias=b1_sb[:, 0:1])
                    nc.vector.scalar_tensor_tensor(
                        out=h1s[:, c0:c0 + 512], in0=h1p[:],
                        scalar=b1_sb[:, 0:1], in1=sg[:, c0:c0 + 512],
                        op0=ALU.add, op1=ALU.mult)
                for c0 in range(0, BP * 128, 512):
                    h2p = h_ps.tile([H, 512], fp32, tag="h")
                    nc.tensor.matmul(h2p[:], w2_sb[:],
                                     h1s[:, c0:c0 + 512], start=True, stop=True)
                    nc.scalar.activation(sg[:, c0:c0 + 512], h2p[:],
                                         AF.Sigmoid, bias=b2_sb[:, 0:1])
                    nc.vector.scalar_tensor_tensor(
                        out=h2s[:, c0:c0 + 512], in0=h2p[:],
                        scalar=b2_sb[:, 0:1], in1=sg[:, c0:c0 + 512],
                        op0=ALU.add, op1=ALU.mult)

                # ---- per-tile edge weights w = h2s_t.T @ W3 (bf16 out) ----
                # one 1-bank PSUM tile per 512-col chunk, double-buffered:
                # tile t+1's matmul overlaps tile t's PSUM->SBUF copy
                w_sb = wp.tile([128, BP, W3COL], bf16, tag="wsb")
                for bt in range(BP):
                    for c0 in range(0, W3COL, 512):
                        c1 = min(c0 + 512, W3COL)
                        w_ps = w_ps_pool.tile([128, 512], fp32, tag="wps")
                        nc.tensor.matmul(w_ps[:, 0:c1 - c0],
                                         h2s[:, bt * 128:(bt + 1) * 128],
                                         w3_sb[:, c0:c1], start=True, stop=True)
                        # L1 is ScalarE-gated: split PSUM->SBUF copies with
                        # DVE (which idles ~65% there); L2 keeps ScalarE
                        # (DVE is the L2 bottleneck)
                        if layer2 or bt % 2 == 0:
                            nc.scalar.copy(w_sb[:, bt, c0:c1],
                                           w_ps[:, 0:c1 - c0])
                        else:
                            nc.vector.tensor_copy(out=w_sb[:, bt, c0:c1],
                                                  in_=w_ps[:, 0:c1 - c0])

                if STAGE < 3:
                    continue
                # ---- tensor product ----
                msgs = msgp.tile([128, BP, F], bf16, tag="msgs")
                if layer2:
                    _emit_tp_l2(nc, ALU, xg, w_sb, shq_t, msgs, pp, zjp, zgp)
                else:
                    _emit_tp_l1(nc, ALU, xg, w_sb, shB, msgs, pp)

                if debug and g == 0:
                    dxg = pp.tile([128, BP, GCOLP], fp32, tag="dxg")
                    nc.vector.tensor_copy(out=dxg[:], in_=xg[:])
                    nc.sync.dma_start(dbg_xg[:], dxg[:])
                    dw = pp.tile([128, BP, W3COL], fp32, tag="dw")
                    nc.vector.tensor_copy(out=dw[:], in_=w_sb[:])
                    nc.sync.dma_start(dbg_w[:], dw[:])
                    dmg = pp.tile([128, BP, F], fp32, tag="dmg")
                    nc.vector.tensor_copy(out=dmg[:], in_=msgs[:])
                    nc.sync.dma_start(dbg_msgs[:], dmg[:])

                if STAGE < 4:
                    continue
                # ---- segment sum: paired-kg bf16 matmuls, PSUM-accumulated ----
                for bt in range(BP):
                    t = t0 + bt
                    w, t_in_w = t // tpw, t % tpw
                    if w not in agg_open:
                        agg_open[w] = agg_pool.tile([128, 640], fp32,
                                                    name="aggps", tag="aggps")
                    ps = agg_open[w]
                    first, last = t_in_w == 0, t_in_w == tpw - 1
                    for pair in range(5):
                        c0 = pair * 128
                        cw = 128 if pair < 4 else 64
                        nc.tensor.matmul(
                            ps[0:cw, c0:c0 + 128],
                            msgs[:, bt, c0:c0 + cw],
                            smt[:, bt, :],
                            start=first and pair in (0, 4),
                            stop=last, skip_group_check=True)
                    if last:
                        aps = agg_open.pop(w)
                        afp = aggt.tile([128, 640], fp32, tag="afp")
                        nc.scalar.copy(afp[:, 0:512], aps[:, 0:512])
                        nc.scalar.copy(afp[0:64, 512:640], aps[0:64, 512:640])
                        nc.sync.dma_start(
                            agg_sb[:, w, 0, :, :],
                            afp[0:64, :].rearrange("p (q n) -> p q n", q=5))
                        nc.sync.dma_start(
                            agg_sb[:, w, 1, 0:4, :],
                            afp[64:128, 0:512].rearrange("p (q n) -> p q n",
                                                         q=4))



        # ---------------- per-window node update ----------------
        if STAGE < 5:
            with ExitStack() as upctx:
                upt = upctx.enter_context(tc.tile_pool(name="upt", bufs=2))
                for w in range(NW):
                    tmp = upt.tile([64, 9, 128], fp32, tag="pass")
                    nc.sync.dma_start(
                        tmp[:], oldT_d[:, :].rearrange(
                            "p (q n) -> p q n",
                            q=9)[:, :, w * 128:(w + 1) * 128])
                    nc.sync.dma_start(
                        newT_d[:, :].rearrange(
                            "p (q n) -> p q n",
                            q=9)[:, :, w * 128:(w + 1) * 128],
                        tmp[:])
            nc.compile()
            return nc
        with ExitStack() as upctx:
            y_pool = upctx.enter_context(
                tc.tile_pool(name="y_ps", bufs=2, space="PSUM"))
            g_pool = upctx.enter_context(
                tc.tile_pool(name="g_ps", bufs=2, space="PSUM"))
            upt = upctx.enter_context(tc.tile_pool(name="upt", bufs=2))
            for w in range(NW):
                oldw = upt.tile([64, 9, 128], fp32, tag="oldw")
                nc.sync.dma_start(
                    oldw[:], oldT_d[:, :].rearrange(
                        "p (q n) -> p q n", q=9)[:, :, w * 128:(w + 1) * 128])
                neww = upt.tile([64, 9, 128], fp32, tag="neww")
                y_ps = y_pool.tile([64, 9 * 128], fp32, tag="yps")
                for kg in range(9):
                    l = 0 if kg == 0 else (1 if kg <= 3 else 2)
                    nc.tensor.matmul(
                        y_ps[:, kg * 128:(kg + 1) * 128],
                        lin_sb[l][:],
                        agg_sb[:, w, kg % 2, kg // 2, :],
                        start=kg in (0, 4, 8), stop=True,
                        skip_group_check=True)
                y0g = upt.tile([C, 128], fp32, tag="y0g")
                nc.scalar.activation(y0g[:], y_ps[:, 0:128], AF.Sigmoid)
                y0s = upt.tile([C, 128], fp32, tag="y0s")
                nc.vector.tensor_tensor(out=y0s[:], in0=y_ps[:, 0:128],
                                        in1=y0g[:], op=ALU.mult)
                nc.vector.tensor_tensor(out=neww[:, 0, :], in0=y0s[:],
                                        in1=oldw[:, 0, :], op=ALU.add)
                g_ps = g_pool.tile([C, 2, 128], fp32, tag="gps")
                for l in (1, 2):
                    nc.tensor.matmul(g_ps[:, l - 1, :], gw_sb[l - 1][:],
                                     neww[:, 0, :], start=(l == 1), stop=True,
                                     skip_group_check=True)
                gts = upt.tile([C, 2, 128], fp32, tag="gts")
                for l in (1, 2):
                    nc.scalar.activation(gts[:, l - 1, :], g_ps[:, l - 1, :],
                                         AF.Sigmoid, bias=gb_sb[l - 1][:, 0:1])
                gy = upt.tile([C, 8, 128], fp32, tag="gy")
                nc.vector.tensor_tensor(
                    out=gy[:, 0:3, :],
                    in0=y_ps[:].rearrange("p (q n) -> p q n", q=9)[:, 1:4, :],
                    in1=gts[:, 0:1, :].broadcast_to((C, 3, 128)),
                    op=ALU.mult)
                nc.vector.tensor_tensor(
                    out=gy[:, 3:8, :],
                    in0=y_ps[:].rearrange("p (q n) -> p q n", q=9)[:, 4:9, :],
                    in1=gts[:, 1:2, :].broadcast_to((C, 5, 128)),
                    op=ALU.mult)
                nc.vector.tensor_tensor(out=neww[:, 1:9, :], in0=gy[:],
                                        in1=oldw[:, 1:9, :], op=ALU.add)
                nc.sync.dma_start(
                    newT_d[:, :].rearrange("p (q n) -> p q n",
                                           q=9)[:, :, w * 128:(w + 1) * 128],
                    neww[:])

    nc.compile()
    return nc


def _emit_tp_l2(nc, ALU, xg, w_sb, shq_t, msgs, pp, zjp, zgp):
    """P products, cg-folded zjQ scalings, FMA as adds (+8 ratio stts).

    All on DVE: single in-order queue, no cross-engine ping-pong; GpSimd
    has ~9us fixed cost per elementwise op and ScalarE cannot multiply
    two tensors, so the TP bulk lives here.
    """
    import concourse.mybir as mybir
    bf16 = mybir.dt.bfloat16
    fp32 = mybir.dt.float32

    GP_JS = ()          # GpSimd streams ~0.4 elem/cyc and zjG bufs=1
                        # serialized groups -> ~19us DVE stall per group;
                        # cheaper to keep all zjq on DVE (nnz-only now)
    EXPC = 8            # ping-pong: 2 x 8x8x32 fp32 = 16KB
    P0 = pp.tile([128, BP, SCHED2_L2[0]["nblk"] * C], bf16, name="P0", tag="P0")
    P12 = pp.tile([128, BP, SCHED2_L2[2]["nblk"] * C], bf16,
                  name="P12", tag="P12")
    zj12 = zjp.tile([128, BP, SCHED2_L2[2]["nblk"] * C], bf16,
                    name="zj12", tag="zj12")
    exps = [zgp.tile([128, BP, EXPC, C // 2], fp32, name=f"exp{i}",
                     tag=f"exp{i}") for i in range(2)]
    ppg = [0]

    def emit_zjq(eng, dst, P, qoff, segs):
        # shq host-packed as fp32 = (v,v) bf16 pair: ScalarE broadcast-
        # expands at half the element count, DVE bitcasts back to bf16 and
        # multiplies with unit-stride in1 -> 2x_1P mode. 4-deep ping-pong
        # so expansion runs ahead of consumption across FMA phases.
        chunks, cur, used = [], [], 0
        for (z0, L) in segs:
            while L > 0:
                if used == EXPC:
                    chunks.append(cur)
                    cur, used = [], 0
                take = min(L, EXPC - used)
                cur.append((z0, take, used))
                z0, L, used = z0 + take, L - take, used + take
        if cur:
            chunks.append(cur)
        for ch in chunks:
            exp = exps[ppg[0]]
            ppg[0] = (ppg[0] + 1) % 2
            for (z0, L, eo) in ch:
                nc.scalar.copy(
                    exp[:, :, eo:eo + L, :],
                    shq_t[:, :, qoff + z0:qoff + z0 + L, None].broadcast_to(
                        (128, BP, L, C // 2)))
            expb = exp[:].bitcast(mybir.dt.bfloat16)
            for (z0, L, eo) in ch:
                eng.tensor_tensor(
                    out=dst[:].rearrange("p b (n c) -> p b n c",
                                         c=C)[:, :, z0:z0 + L, :],
                    in0=P[:].rearrange("p b (n c) -> p b n c",
                                       c=C)[:, :, z0:z0 + L, :],
                    in1=expb[:, :, eo:eo + L, :],
                    op=ALU.mult)

    def emit_fma(sch, j, zj):
        for (jj, z0, k0, L) in sch["runs_add"]:
            if jj != j:
                continue
            nc.vector.tensor_tensor(
                out=msgs[:, :, k0 * C:(k0 + L) * C],
                in0=zj[:, :, z0 * C:(z0 + L) * C],
                in1=msgs[:, :, k0 * C:(k0 + L) * C],
                op=ALU.add)
        for (jj, z, k, ratio) in sch["runs_stt"]:
            if jj != j:
                continue
            nc.vector.scalar_tensor_tensor(
                out=msgs[:, :, k * C:(k + 1) * C],
                in0=zj[:, :, z * C:(z + 1) * C],
                scalar=float(ratio),
                in1=msgs[:, :, k * C:(k + 1) * C],
                op0=ALU.mult, op1=ALU.add)

    for gi, sch in enumerate(SCHED2_L2):
        nblk = sch["nblk"]
        P = P0 if gi == 0 else P12
        for (p, ig0, ni, z0) in sch["pops"]:
            nc.vector.tensor_tensor(
                out=P[:].rearrange("p b (n c) -> p b n c",
                                   c=C)[:, :, z0:z0 + ni, :],
                in0=xg[:].rearrange("p b (n c) -> p b n c",
                                    c=C)[:, :, ig0:ig0 + ni, :],
                in1=w_sb[:, :, None, p * C:(p + 1) * C].broadcast_to(
                    (128, BP, ni, C)),
                op=ALU.mult)
        if gi > 0:
            # kick GpSimd's share first so it overlaps DVE's other j's
            for (j, qoff, segs) in sch["zjq"]:
                if j in GP_JS:
                    emit_zjq(nc.gpsimd, zjG[j], P, qoff, segs)
        for (j, qoff, segs) in sch["zjq"]:
            if gi > 0 and j in GP_JS:
                continue
            dst = msgs if gi == 0 else zj12
            emit_zjq(nc.vector, dst, P, qoff, segs)
            if gi == 0:
                continue
            emit_fma(sch, j, zj12)
        if gi > 0:
            for j in GP_JS:
                if any(jj == j for (jj, _, _) in sch["zjq"]):
                    emit_fma(sch, j, zjG[j])


def _emit_tp_l1(nc, ALU, xg, w_sb, shB, msgs, pp):
    """msgs[k] = x * w'_k * sh_k ; w' host-expanded+cg-folded to 9 kg blocks."""
    import concourse.mybir as mybir
    bf16 = mybir.dt.bfloat16
    tmp = pp.tile([128, BP, 9, C], bf16, name="l1tmp", tag="l1tmp")
    nc.vector.tensor_tensor(
        out=tmp[:],
        in0=w_sb[:].rearrange("p b (n c) -> p b n c", c=C),
        in1=shB[:].bitcast(bf16),
        op=ALU.mult)
    nc.vector.tensor_tensor(
        out=msgs[:].rearrange("p b (n c) -> p b n c", c=C),
        in0=tmp[:],
        in1=xg[:, :, None, 0:C].broadcast_to((128, BP, 9, C)),
        op=ALU.mult)


# ---------------- host orchestration ----------------
def _chunked_T(feats_own):
    """[NPC, 576] -> kg-blocked transposed [64, 9*NPC]."""
    out = np.empty((64, 9 * NPC), np.float32)
    for kg in range(9):
        out[:, kg * NPC:(kg + 1) * NPC] = feats_own[:, kg * 64:(kg + 1) * 64].T
    return out


def _unchunk_T(newT):
    """[64, 9*NPC] -> [NPC, 576]."""
    out = np.empty((NPC, 576), np.float32)
    for kg in range(9):
        out[:, kg * 64:(kg + 1) * 64] = newT[:, kg * NPC:(kg + 1) * NPC].T
    return out


_CACHE = {}


def _prep(positions, senders, receivers):
    key = (senders.tobytes(), receivers.tobytes(), positions.tobytes())
    if _CACHE.get("key") == key:
        return _CACHE["val"]
    sh_eff, basis = edge_geometry(positions, senders, receivers)
    owner, local, nodes_of, _ = partition_graph(receivers)
    deg_bin = np.zeros(NCORES * NW, np.int64)
    np.add.at(deg_bin, owner[receivers] * NW + local[receivers] // 128, 1)
    tpw = (int(deg_bin.max()) + 127) // 128
    T = NW * tpw
    assert T % BP == 0
    perm = build_core_edges(receivers, owner, local, tpw)

    valid = perm >= 0
    eg = np.where(valid, perm, 0)
    snd = np.where(valid, senders[eg], 0).astype(np.int16)      # [NC, T*128]
    shp_e = sh_eff[eg] * valid[..., None]                        # [NC, T*128, 9]
    bas_e = basis[eg] * valid[..., None]                         # [NC, T*128, 8]
    lr = np.where(valid, local[receivers[eg]], 0)

    NG = T // BP
    inv = np.float32(1.0 / np.sqrt(AVG_NN))
    sidx = np.empty((NCORES, 128, T * 128 // 16), np.int16)
    shp_h = np.empty((NCORES, 128, T, 9), np.float32)
    shq_h = np.empty((NCORES, 128, T, NSHQ), np.float32)
    jcols = np.array([j for (j, cg) in SHQ_COLS])
    cgv = np.array([cg for (j, cg) in SHQ_COLS], np.float32)
    bas_h = np.empty((NCORES, NG, 8, BP * 128), BF16)
    smat_h = np.zeros((NCORES, 128, T, 128), BF16)
    for k in range(NCORES):
        s = snd[k].reshape(T * 8, 16)
        sidx[k] = np.tile(s.T, (8, 1))
        shp_f = shp_e[k].reshape(T, 128, 9).transpose(1, 0, 2)
        # fp32-packed (v, v) bf16 pair so on-chip broadcast-expansion moves
        # half the elements (bitcast back to bf16 at the consumer)
        up = shp_f.astype(BF16).view(np.uint16).astype(np.uint32)
        shp_h[k] = ((up << 16) | up).view(np.float32)
        u = (shp_f[:, :, jcols] * cgv[None, None, :]).astype(
            BF16).view(np.uint16).astype(np.uint32)
        shq_h[k] = ((u << 16) | u).view(np.float32)
        bas_h[k] = bas_e[k].reshape(NG, BP * 128, 8).transpose(0, 2, 1).astype(BF16)
        v = valid[k]
        e_slots = np.arange(T * 128)
        p_, t_ = e_slots % 128, e_slots // 128
        cols = lr[k] - (t_ // tpw) * 128
        ok = v & (cols >= 0) & (cols < 128)
        sm = np.zeros((128, T, 128), np.float32)
        sm[p_[ok], t_[ok], cols[ok]] = inv
        smat_h[k] = sm.astype(BF16)
    val = dict(T=T, NG=NG, tpw=tpw, nodes_of=nodes_of, sidx=sidx,
               shp_h=shp_h, bas_h=bas_h, smat_h=smat_h, shq_h=shq_h)
    _CACHE["key"], _CACHE["val"] = key, val
    return val


EXEC_NS = []


def _run_layer(nc, pre, ftab, oldT_by_core, lw):
    import os
    from concourse.bass_utils import run_bass_kernel_spmd
    in_maps = []
    for k in range(NCORES):
        m = dict(ftab=ftab,
                 sidx=pre["sidx"][k],
                 shp=pre["shp_h"][k],
                 shq=pre["shq_h"][k],
                 basisT=pre["bas_h"][k],
                 smat=pre["smat_h"][k],
                 oldT=oldT_by_core[k],
                 w1=lw["w1"], b1=lw["b1"], w2=lw["w2"], b2=lw["b2"],
                 w3=lw["w3"], lin0=lw["lin"][0], lin1=lw["lin"][1],
                 lin2=lw["lin"][2], gw0=lw["gw"][0], gw1=lw["gw"][1],
                 gb0=lw["gb"][0], gb1=lw["gb"][1])
        in_maps.append(m)
    trace = bool(os.environ.get("KERNEL_TRACE"))
    res = run_bass_kernel_spmd(nc, in_maps, list(range(NCORES)), trace=trace,
                               trace_cores=list(range(NCORES)) if trace else None)
    if trace and res.exec_time_ns is not None:
        EXEC_NS.append(res.exec_time_ns)
    return [res.results[k]["newT"] for k in range(NCORES)]


def _layer_weights(inputs, i, layer2):
    f32 = np.float32
    w3 = np.ascontiguousarray(inputs["mlp_w3"][i], f32)
    if layer2:
        w3p = w3.astype(BF16)
    else:
        w3p = np.empty((H, 576), BF16)
        for kg in range(9):
            p = L1_PATH_OF_K[kg]
            w3p[:, kg * C:(kg + 1) * C] = (
                w3[:, p * C:(p + 1) * C] * L1_CG_OF_K[kg]).astype(BF16)
    return dict(
        w1=np.ascontiguousarray(inputs["mlp_w1"][i], f32).astype(BF16),
        b1=np.ascontiguousarray(inputs["mlp_b1"][i], f32).reshape(H, 1),
        w2=np.ascontiguousarray(inputs["mlp_w2"][i], f32).astype(BF16),
        b2=np.ascontiguousarray(inputs["mlp_b2"][i], f32).reshape(H, 1),
        w3=w3p,
        lin=[np.ascontiguousarray(inputs["lin_self"][i, l], f32)
             for l in range(3)],
        gw=[np.ascontiguousarray(inputs["gate_w"][i, l], f32)
            for l in range(2)],
        gb=[np.ascontiguousarray(inputs["gate_b"][i, l], f32).reshape(C, 1)
            for l in range(2)],
    )


_KERNEL_CACHE = {}


def _get_kernels(T):
    if T not in _KERNEL_CACHE:
        _KERNEL_CACHE[T] = (build_layer_kernel(False, T),
                            build_layer_kernel(True, T))
    return _KERNEL_CACHE[T]


def _pack_ftab(table, ncols):
    out = np.zeros((N_NODES, ncols), BF16)
    used = min(ncols, table.shape[1])
    out[:, :used] = table[:, :used].astype(BF16)
    return out


def kernel(**inputs):
    positions = np.asarray(inputs["positions"], np.float32)
    species = np.asarray(inputs["species"]).astype(np.int64)
    senders = np.asarray(inputs["senders"]).astype(np.int64)
    receivers = np.asarray(inputs["receivers"]).astype(np.int64)

    pre = _prep(positions, senders, receivers)
    T = pre["T"]
    nc1, nc2 = _get_kernels(T)
    nodes_of = pre["nodes_of"]

    # initial features: x0 from species embedding (host; tiny)
    x0 = (np.asarray(inputs["embed"], np.float32)[species]
          @ np.asarray(inputs["w_proj"], np.float32))          # [N, 64]
    table = np.zeros((N_NODES, F), np.float32)
    table[:, 0:C] = x0

    # ---- layer 1 ----
    oldT = [_chunked_T(table[nodes_of[k]]) for k in range(NCORES)]
    lw = _layer_weights(inputs, 0, False)
    newT = _run_layer(nc1, pre, _pack_ftab(table, GCOLP_L1), oldT, lw)

    table2 = np.empty((N_NODES, F), np.float32)
    for k in range(NCORES):
        table2[nodes_of[k]] = _unchunk_T(newT[k])

    # ---- layer 2 ----
    lw = _layer_weights(inputs, 1, True)
    newT2 = _run_layer(nc2, pre, _pack_ftab(table2, GCOLP_L2), newT, lw)

    table3 = np.empty((N_NODES, F), np.float32)
    for k in range(NCORES):
        table3[nodes_of[k]] = _unchunk_T(newT2[k])

    # ---- output: reorder component-major -> reference layout + alpha ----
    t3 = table3.reshape(N_NODES, 9, C)
    out = np.empty((N_NODES, F), np.float32)
    out[:, 0:64] = t3[:, 0]
    out[:, 64:256] = (0.5 * t3[:, 1:4]).transpose(0, 2, 1).reshape(N_NODES, 192)
    out[:, 256:576] = (0.25 * t3[:, 4:9]).transpose(0, 2, 1).reshape(N_NODES, 320)
    return out



# revision 26
# speedup vs baseline: 1.3664x; 1.0091x over previous
"""NequIP GNN message-passing kernel for 8 Trainium2 NeuronCores — v2.

Receiver-sharded graph parallelism (per sharding hint): host LPT-assigns the
8192 nodes to 64 (core, window) bins of 128, each core owns 8 windows = 1024
nodes + their in-edges, sorted by window, padded to 128-edge tiles.

v2 device pipeline (vs v1): bf16 edge pipeline end-to-end with fp32 PSUM
accumulation; per-edge-scalar stages batched across 8-tile groups via
stride-0 broadcast access patterns; P-products collapsed per-path (w operand
broadcast over l1 components); CG-FMA stage merged into diagonal runs with
immediate scalars; segment-sum via paired-kg bf16 matmuls; radial MLP with
fused Silu activations; elementwise work split across Vector/GpSimd/Scalar.
"""
import math
import numpy as np
import ml_dtypes

BF16 = ml_dtypes.bfloat16

# ---------------- model constants ----------------
N_NODES, N_EDGES = 8192, 131072
C, H, NRAD = 64, 64, 8
R_MAX, AVG_NN = 5.0, 16.0
NCORES, NPC = 8, 1024
NW = NPC // 128
F = 9 * C
LS = (0, 1, 2)
PATHS = [(l1, l2, l3) for l1 in LS for l2 in LS for l3 in LS
         if abs(l1 - l2) <= l3 <= l1 + l2]
LOFF = {0: 0, 1: 1, 2: 4}
J_OF_L2 = {0: [0], 1: [1, 2, 3], 2: [4, 5, 6, 7, 8]}
BP = 8                      # tiles per group (batched in op free dims)
GCOLP_L2 = 640              # padded gather row (bf16): 1280B % 256 == 0
GCOLP_L1 = 128              # 256B % 256 == 0


# ---------------- real Clebsch-Gordan coefficients ----------------
def _cg_scalar(j1, m1, j2, m2, j3, m3):
    f = math.factorial
    if m1 + m2 != m3:
        return 0.0
    pre = ((2*j3+1) * f(j1+j2-j3) * f(j1-j2+j3) * f(-j1+j2+j3)
           / f(j1+j2+j3+1)) ** 0.5
    pre *= (f(j1+m1)*f(j1-m1)*f(j2+m2)*f(j2-m2)*f(j3+m3)*f(j3-m3)) ** 0.5
    s = 0.0
    for k in range(max(0, j2-j3-m1, j1+m2-j3), min(j1+j2-j3, j1-m1, j2+m2)+1):
        s += (-1)**k / (f(k)*f(j1+j2-j3-k)*f(j1-m1-k)
                        * f(j2+m2-k)*f(j3-j2+m1+k)*f(j3-j1-m2+k))
    return pre * s


def _U_real(l):
    U = np.zeros((2*l+1, 2*l+1), dtype=complex)
    s2 = 2 ** -0.5
    for m in range(-l, l+1):
        if m > 0:
            U[m+l, m+l] = (-1)**m * s2
            U[m+l, -m+l] = s2
        elif m == 0:
            U[l, l] = 1.0
        else:
            U[m+l, m+l] = 1j*s2
            U[m+l, -m+l] = -1j*(-1)**(-m)*s2
    return U


def _real_cg(l1, l2, l3):
    Cc = np.zeros((2*l1+1, 2*l2+1, 2*l3+1))
    for i1, m1 in enumerate(range(-l1, l1+1)):
        for i2, m2 in enumerate(range(-l2, l2+1)):
            m3 = m1 + m2
            if abs(m3) <= l3:
                Cc[i1, i2, m3+l3] = _cg_scalar(l1, m1, l2, m2, l3, m3)
    U1, U2, U3 = _U_real(l1), _U_real(l2), _U_real(l3)
    W = np.einsum('ia,jb,kc,abc->ijk', U1.conj(), U2.conj(), U3,
                  Cc.astype(complex))
    W = W.real if np.linalg.norm(W.real) >= np.linalg.norm(W.imag) else W.imag
    W = W / np.linalg.norm(W) * (2*l3+1) ** 0.5
    return np.asarray(W, dtype=np.float64)


CGS = [_real_cg(*p) for p in PATHS]


def build_schedule_l2():
    """Static TP structure for layer 2, grouped by l2.

    Per group: nblk, pops=[(path, ig0, ni, z0)], jlist, runs=[(j,z0,k0,L,cg)].
    """
    scheds = []
    for l2 in (0, 1, 2):
        ps = [p for p in range(len(PATHS)) if PATHS[p][1] == l2]
        blocks, block_of, pops = [], {}, []
        for p in ps:
            l1 = PATHS[p][0]
            ni = 2 * l1 + 1
            pops.append((p, LOFF[l1], ni, len(blocks)))
            for i in range(ni):
                block_of[(p, i)] = len(blocks)
                blocks.append((p, i))
        tset = set()
        for p in ps:
            l1, _, l3 = PATHS[p]
            cg = CGS[p]
            for i in range(2*l1+1):
                for j in range(2*l2+1):
                    for k in range(2*l3+1):
                        v = cg[i, j, k]
                        if abs(v) > 1e-12:
                            tset.add((LOFF[l2]+j, block_of[(p, i)],
                                      LOFF[l3]+k, round(float(v), 9)))
        runs, consumed = [], set()
        for t in sorted(tset):
            if t in consumed:
                continue
            j, z, k, cgv = t
            L = 0
            while (j, z+L, k+L, cgv) in tset and (j, z+L, k+L, cgv) not in consumed:
                consumed.add((j, z+L, k+L, cgv))
                L += 1
            runs.append((j, z, k, L, cgv))
        scheds.append(dict(l2=l2, nblk=len(blocks), pops=pops,
                           jlist=J_OF_L2[l2], runs=runs))
    return scheds


SCHED_L2 = build_schedule_l2()


def build_schedule2():
    """cg folded into per-(j,z) host scalars; FMA mostly tensor adds.

    Per group: nblk, pops, zjq=[(j, qoff)], runs_add=[(j,z0,k0,L)],
    runs_stt=[(j,z,k,ratio)]. qoff = column offset of (j,*) block in the
    concatenated shq table [128, T, 179]. cgfold[qoff+z] = cg of the primary
    (smallest-k) triple of (j,z); 0 for unused (z,j).
    """
    scheds, qoff, cgcols = [], 0, []
    for gi, sch in enumerate(SCHED_L2):
        nblk = sch["nblk"]
        tset = set()
        for (j, z, k, L, cg) in sch["runs"]:
            for i in range(L):
                tset.add((j, z + i, k + i, cg))
        per_jz = {}
        for (j, z, k, cg) in tset:
            per_jz.setdefault((j, z), []).append((k, cg))
        primary, runs_stt = set(), []
        cgf = {}
        for (j, z), ks in per_jz.items():
            ks.sort()
            k0, cg0 = ks[0]
            primary.add((j, z, k0))
            cgf[(j, z)] = cg0
            for (k1, cg1) in ks[1:]:
                runs_stt.append((j, z, k1, cg1 / cg0))
        runs_add, consumed = [], set()
        for t in sorted(primary):
            if t in consumed:
                continue
            j, z, k = t
            L = 0
            while (j, z + L, k + L) in primary and (j, z + L, k + L) not in consumed:
                consumed.add((j, z + L, k + L))
                L += 1
            runs_add.append((j, z, k, L))
        zjq = []
        for j in sch["jlist"]:
            # contiguous segments of z columns actually used by (j, z):
            # skips the ~28% of dense columns with cg == 0
            used = sorted(z for (jj, z) in cgf if jj == j)
            segs, s0, prev = [], None, None
            for z in used:
                if s0 is None:
                    s0 = prev = z
                elif z == prev + 1:
                    prev = z
                else:
                    segs.append((s0, prev - s0 + 1))
                    s0 = prev = z
            if s0 is not None:
                segs.append((s0, prev - s0 + 1))
            zjq.append((j, qoff, segs))
            for z in range(nblk):
                cgcols.append((j, cgf.get((j, z), 0.0)))
            qoff += nblk
        scheds.append(dict(nblk=nblk, pops=sch["pops"], zjq=zjq,
                           runs_add=runs_add, runs_stt=runs_stt))
    return scheds, cgcols


SCHED2_L2, SHQ_COLS = build_schedule2()
NSHQ = len(SHQ_COLS)

# layer-1 per-k path and cg (x is scalar-only: paths (0,l,l), j == k)
L1_PATH_OF_K = [0] + [1]*3 + [2]*5
L1_CG_OF_K = []
for _k in range(9):
    _p = L1_PATH_OF_K[_k]
    _l = PATHS[_p][2]
    _m = _k - LOFF[_l]
    L1_CG_OF_K.append(float(CGS[_p][0, _m, _m]))


# ---------------- host-side graph preprocessing ----------------
def edge_geometry(positions, senders, receivers):
    rel = (positions[receivers] - positions[senders]) / R_MAX
    d = np.linalg.norm(rel, axis=-1)
    u = rel / np.maximum(d, 1e-6)[:, None]
    x, y, z = u[:, 0], u[:, 1], u[:, 2]
    sh = np.empty((len(d), 9), np.float32)
    sh[:, 0] = 1.0
    sh[:, 1] = np.sqrt(3.0) * y
    sh[:, 2] = np.sqrt(3.0) * z
    sh[:, 3] = np.sqrt(3.0) * x
    sh[:, 4] = np.sqrt(15.0) * x * y
    sh[:, 5] = np.sqrt(15.0) * y * z
    sh[:, 6] = np.sqrt(5.0) / 2 * (3 * z * z - 1.0)
    sh[:, 7] = np.sqrt(15.0) * x * z
    sh[:, 8] = np.sqrt(15.0) / 2 * (x * x - y * y)
    freqs = np.arange(1, NRAD + 1, dtype=np.float64)
    xr = np.clip(d, 1e-4, 1.0)[:, None].astype(np.float64)
    basis = (np.sqrt(2.0) * np.sin(freqs * np.pi * xr) / xr).astype(np.float32)
    cut = (0.5 * (np.cos(np.pi * np.clip(d, 0.0, 1.0)) + 1.0)).astype(np.float32)
    return (sh * cut[:, None]).astype(np.float32), basis


def partition_graph(receivers):
    import heapq
    deg = np.bincount(receivers, minlength=N_NODES)
    order = np.argsort(-deg, kind="stable")
    nbins = NCORES * NW
    load = np.zeros(nbins, np.int64)
    cnt = np.zeros(nbins, np.int64)
    owner = np.empty(N_NODES, np.int32)
    local = np.empty(N_NODES, np.int32)
    heap = [(0, b) for b in range(nbins)]
    heapq.heapify(heap)
    for n in order:
        while True:
            l, b = heapq.heappop(heap)
            if cnt[b] < 128:
                break
        owner[n] = b // NW
        local[n] = (b % NW) * 128 + cnt[b]
        cnt[b] += 1
        load[b] += deg[n]
        if cnt[b] < 128:
            heapq.heappush(heap, (int(load[b]), b))
    nodes_of = np.empty((NCORES, NPC), np.int64)
    for n in range(N_NODES):
        nodes_of[owner[n], local[n]] = n
    return owner, local, nodes_of, int(load.max())


def build_core_edges(receivers, owner, local, tpw):
    T = NW * tpw
    perm = np.full((NCORES, T * 128), -1, np.int64)
    for k in range(NCORES):
        eids = np.where(owner[receivers] == k)[0]
        lr = local[receivers[eids]]
        o = np.argsort(lr, kind="stable")
        eids, lr = eids[o], lr[o]
        w_of = lr // 128
        for w in range(NW):
            sel = eids[w_of == w]
            assert len(sel) <= tpw * 128, "tiles-per-window overflow"
            base = w * tpw * 128
            perm[k, base:base + len(sel)] = sel
    return perm


# ---------------- bass kernel builder ----------------
def build_layer_kernel(layer2, T, debug=False):
    import concourse.bass as bass
    import concourse.bacc as bacc
    import concourse.tile as tile
    import concourse.mybir as mybir
    from contextlib import ExitStack

    fp32 = mybir.dt.float32
    bf16 = mybir.dt.bfloat16
    AF = mybir.ActivationFunctionType
    ALU = mybir.AluOpType

    NPATH = 15 if layer2 else 3
    GCOLP = GCOLP_L2 if layer2 else GCOLP_L1
    W3COL = NPATH * C if layer2 else 576   # L1 w3 host-expanded to 9 kg blocks
    E_PAD = T * 128
    NG = T // BP
    assert T % BP == 0 and T % NW == 0
    tpw = T // NW

    import os as _os
    STAGE = int(_os.environ.get("KV2_STAGE", "5"))
    nc = bacc.Bacc("TRN2", target_bir_lowering=False)

    ftab = nc.dram_tensor("ftab", [N_NODES, GCOLP], bf16, kind="ExternalInput")
    sidx = nc.dram_tensor("sidx", [128, E_PAD // 16], mybir.dt.int16,
                          kind="ExternalInput")
    shp_d = nc.dram_tensor("shp", [128, T, 9], fp32, kind="ExternalInput")
    shq_d = nc.dram_tensor("shq", [128, T, NSHQ], fp32, kind="ExternalInput")
    basT_d = nc.dram_tensor("basisT", [NG, 8, BP * 128], bf16,
                            kind="ExternalInput")
    smat_d = nc.dram_tensor("smat", [128, T, 128], bf16, kind="ExternalInput")
    oldT_d = nc.dram_tensor("oldT", [64, 9 * NPC], fp32, kind="ExternalInput")
    w1_d = nc.dram_tensor("w1", [8, H], bf16, kind="ExternalInput")
    b1_d = nc.dram_tensor("b1", [H, 1], fp32, kind="ExternalInput")
    w2_d = nc.dram_tensor("w2", [H, H], bf16, kind="ExternalInput")
    b2_d = nc.dram_tensor("b2", [H, 1], fp32, kind="ExternalInput")
    w3_d = nc.dram_tensor("w3", [H, W3COL], bf16, kind="ExternalInput")
    lin_d = [nc.dram_tensor(f"lin{l}", [C, C], fp32, kind="ExternalInput")
             for l in range(3)]
    gw_d = [nc.dram_tensor(f"gw{l}", [C, C], fp32, kind="ExternalInput")
            for l in range(2)]
    gb_d = [nc.dram_tensor(f"gb{l}", [C, 1], fp32, kind="ExternalInput")
            for l in range(2)]
    newT_d = nc.dram_tensor("newT", [64, 9 * NPC], fp32,
                            kind="ExternalOutput")
    if debug:
        dbg_xg = nc.dram_tensor("dbg_xg", [128, BP, GCOLP], fp32,
                                kind="ExternalOutput")
        dbg_w = nc.dram_tensor("dbg_w", [128, BP, W3COL], fp32,
                               kind="ExternalOutput")
        dbg_msgs = nc.dram_tensor("dbg_msgs", [128, BP, F], fp32,
                                  kind="ExternalOutput")
        dbg_agg = nc.dram_tensor("dbg_agg", [128, NW, 640], fp32,
                                 kind="ExternalOutput")

    with tile.TileContext(nc) as tc, ExitStack() as ctx:
        consts = ctx.enter_context(tc.tile_pool(name="consts", bufs=1))
        idx_sb = consts.tile([128, E_PAD // 16], mybir.dt.int16)
        nc.sync.dma_start(idx_sb[:], sidx[:])
        w1_sb = consts.tile([8, H], bf16)
        nc.sync.dma_start(w1_sb[:], w1_d[:])
        b1_sb = consts.tile([H, 1], fp32)
        nc.sync.dma_start(b1_sb[:], b1_d[:])
        w2_sb = consts.tile([H, H], bf16)
        nc.sync.dma_start(w2_sb[:], w2_d[:])
        b2_sb = consts.tile([H, 1], fp32)
        nc.sync.dma_start(b2_sb[:], b2_d[:])
        w3_sb = consts.tile([H, W3COL], bf16)
        nc.sync.dma_start(w3_sb[:], w3_d[:])
        lin_sb = [consts.tile([C, C], fp32, name=f"lin{l}", tag=f"lin{l}")
                  for l in range(3)]
        for l in range(3):
            nc.sync.dma_start(lin_sb[l][:], lin_d[l][:])
        gw_sb = [consts.tile([C, C], fp32, name=f"gw{l}", tag=f"gw{l}")
                 for l in range(2)]
        gb_sb = [consts.tile([C, 1], fp32, name=f"gb{l}", tag=f"gb{l}")
                 for l in range(2)]
        for l in range(2):
            nc.sync.dma_start(gw_sb[l][:], gw_d[l][:])
            nc.sync.dma_start(gb_sb[l][:], gb_d[l][:])
        agg_sb = consts.tile([64, NW, 2, 5, 128], fp32)

        with ExitStack() as psctx:
            iop = psctx.enter_context(
                tc.tile_pool(name="iop", bufs=2 if layer2 else 3))
            aggt = psctx.enter_context(tc.tile_pool(name="aggt", bufs=1))
            wp = psctx.enter_context(tc.tile_pool(name="wp", bufs=1))
            msgp = psctx.enter_context(tc.tile_pool(name="msgp", bufs=2))
            shbp = psctx.enter_context(tc.tile_pool(name="shbp", bufs=1))
            pp = psctx.enter_context(tc.tile_pool(name="pp", bufs=1))
            zjp = psctx.enter_context(tc.tile_pool(name="zjp", bufs=1))
            zgp = psctx.enter_context(tc.tile_pool(name="zgp", bufs=1))
            h_ps = psctx.enter_context(
                tc.tile_pool(name="h_ps", bufs=2, space="PSUM"))
            w_ps_pool = psctx.enter_context(
                tc.tile_pool(name="w_ps", bufs=2, space="PSUM"))
            agg_pool = psctx.enter_context(
                tc.tile_pool(name="agg_ps", bufs=2, space="PSUM"))

            agg_open = {}

            for g in range(NG):
                t0 = g * BP
                xg = iop.tile([128, BP, GCOLP], bf16, tag="xg")
                nc.gpsimd.dma_gather(
                    out_ap=xg[:],
                    in_ap=ftab[:],
                    idxs_ap=idx_sb[:, g * (BP * 8):(g + 1) * (BP * 8)],
                    num_idxs=BP * 128,
                    num_idxs_reg=BP * 128,
                    elem_size=GCOLP,
                )
                if not layer2:
                    shp_t = iop.tile([128, BP, 9], fp32, tag="shp")
                    nc.sync.dma_start(shp_t[:], shp_d[:, t0:t0 + BP, :])
                if layer2:
                    shq_t = iop.tile([128, BP, NSHQ], fp32, tag="shq")
                    nc.sync.dma_start(shq_t[:], shq_d[:, t0:t0 + BP, :])
                smt = iop.tile([128, BP, 128], bf16, tag="smt")
                nc.sync.dma_start(smt[:], smat_d[:, t0:t0 + BP, :])
                bas = iop.tile([8, BP * 128], bf16, tag="bas")
                nc.sync.dma_start(bas[:], basT_d[g, :, :])

                if STAGE < 2:
                    continue
                if not layer2:
                    # sh broadcast table for L1: fp32 = packed (v,v) bf16
                    # pair, so ScalarE moves half the elements (1x mode)
                    shB = shbp.tile([128, BP, 9, C // 2], fp32, tag="shB")
                    nc.scalar.copy(
                        shB[:],
                        shp_t[:, :, :, None].broadcast_to(
                            (128, BP, 9, C // 2)))

                # ---- radial MLP (transposed; fused Silu) ----
                h1s = iop.tile([H, BP * 128], bf16, tag="h1s")
                h2s = iop.tile([H, BP * 128], bf16, tag="h2s")
                sg = iop.tile([H, BP * 128], bf16, tag="sg")
                for c0 in range(0, BP * 128, 512):
                    h1p = h_ps.tile([H, 512], fp32, tag="h")
                    nc.tensor.matmul(h1p[:], w1_sb[:],
                                     bas[:, c0:c0 + 512], start=True, stop=True)
                    nc.scalar.activation(sg[:, c0:c0 + 512], h1p[:],
                                         AF.Sigmoid, bias=b1_sb[:, 0:1])
                    nc.vector.scalar_tensor_tensor(
                        out=h1s[:, c0:c0 + 512], in0=h1p[:],
                        scalar=b1_sb[:, 0:1], in1=sg[:, c0:c0 + 512],
                        op0=ALU.add, op1=ALU.mult)
                for c0 in range(0, BP * 128, 512):
                    h2p = h_ps.tile([H, 512], fp32, tag="h")
                    nc.tensor.matmul(h2p[:], w2_sb[:],
                                     h1s[:, c0:c0 + 512], start=True, stop=True)
                    nc.scalar.activation(sg[:, c0:c0 + 512], h2p[:],
                                         AF.Sigmoid, bias=b2_sb[:, 0:1])
                    nc.vector.scalar_tensor_tensor(
                        out=h2s[:, c0:c0 + 512], in0=h2p[:],
                        scalar=b2_sb[:, 0:1], in1=sg[:, c0:c0 + 512],
                        op0=ALU.add, op1=ALU.mult)

                # ---- per-tile edge weights w = h2s_t.T @ W3 (bf16 out) ----
                # one 1-bank PSUM tile per 512-col chunk, double-buffered:
                # tile t+1's matmul overlaps tile t's PSUM->SBUF copy
                w_sb = wp.tile([128, BP, W3COL], bf16, tag="wsb")
                for bt in range(BP):
                    for c0 in range(0, W3COL, 512):
                        c1 = min(c0 + 512, W3COL)
                        w_ps = w_ps_pool.tile([128, 512], fp32, tag="wps")
                        nc.tensor.matmul(w_ps[:, 0:c1 - c0],
                                         h2s[:, bt * 128:(bt + 1) * 128],
                                         w3_sb[:, c0:c1], start=True, stop=True)
                        # L1 is ScalarE-gated: split PSUM->SBUF copies with
                        # DVE (which idles ~65% there); L2 keeps ScalarE
                        # (DVE is the L2 bottleneck)
                        if layer2 or bt % 2 == 0:
                            nc.scalar.copy(w_sb[:, bt, c0:c1],
                                           w_ps[:, 0:c1 - c0])
                        else:
                            nc.vector.tensor_copy(out=w_sb[:, bt, c0:c1],
                                                  in_=w_ps[:, 0:c1 - c0])

                if STAGE < 3:
                    continue
                # ---- tensor product ----
                msgs = msgp.tile([128, BP, F], bf16, tag="msgs")
                if layer2:
                    _emit_tp_l2(nc, ALU, xg, w_sb, shq_t, msgs, pp, zjp, zgp)
                else:
                    _emit_tp_l1(nc, ALU, xg, w_sb, shB, msgs, pp)

                if debug and g == 0:
                    dxg = pp.tile([128, BP, GCOLP], fp32, tag="dxg")
                    nc.vector.tensor_copy(out=dxg[:], in_=xg[:])
                    nc.sync.dma_start(dbg_xg[:], dxg[:])
                    dw = pp.tile([128, BP, W3COL], fp32, tag="dw")
                    nc.vector.tensor_copy(out=dw[:], in_=w_sb[:])
                    nc.sync.dma_start(dbg_w[:], dw[:])
                    dmg = pp.tile([128, BP, F], fp32, tag="dmg")
                    nc.vector.tensor_copy(out=dmg[:], in_=msgs[:])
                    nc.sync.dma_start(dbg_msgs[:], dmg[:])

                if STAGE < 4:
                    continue
                # ---- segment sum: paired-kg bf16 matmuls, PSUM-accumulated ----
                for bt in range(BP):
                    t = t0 + bt
                    w, t_in_w = t // tpw, t % tpw
                    if w not in agg_open:
                        agg_open[w] = agg_pool.tile([128, 640], fp32,
                                                    name="aggps", tag="aggps")
                    ps = agg_open[w]
                    first, last = t_in_w == 0, t_in_w == tpw - 1
                    for pair in range(5):
                        c0 = pair * 128
                        cw = 128 if pair < 4 else 64
                        nc.tensor.matmul(
                            ps[0:cw, c0:c0 + 128],
                            msgs[:, bt, c0:c0 + cw],
                            smt[:, bt, :],
                            start=first and pair in (0, 4),
                            stop=last, skip_group_check=True)
                    if last:
                        aps = agg_open.pop(w)
                        afp = aggt.tile([128, 640], fp32, tag="afp")
                        nc.scalar.copy(afp[:, 0:512], aps[:, 0:512])
                        nc.scalar.copy(afp[0:64, 512:640], aps[0:64, 512:640])
                        nc.sync.dma_start(
                            agg_sb[:, w, 0, :, :],
                            afp[0:64, :].rearrange("p (q n) -> p q n", q=5))
                        nc.sync.dma_start(
                            agg_sb[:, w, 1, 0:4, :],
                            afp[64:128, 0:512].rearrange("p (q n) -> p q n",
                                                         q=4))



        # ---------------- per-window node update ----------------
        if STAGE < 5:
            with ExitStack() as upctx:
                upt = upctx.enter_context(tc.tile_pool(name="upt", bufs=2))
                for w in range(NW):
                    tmp = upt.tile([64, 9, 128], fp32, tag="pass")
                    nc.sync.dma_start(
                        tmp[:], oldT_d[:, :].rearrange(
                            "p (q n) -> p q n",
                            q=9)[:, :, w * 128:(w + 1) * 128])
                    nc.sync.dma_start(
                        newT_d[:, :].rearrange(
                            "p (q n) -> p q n",
                            q=9)[:, :, w * 128:(w + 1) * 128],
                        tmp[:])
            nc.compile()
            return nc
        with ExitStack() as upctx:
            y_pool = upctx.enter_context(
                tc.tile_pool(name="y_ps", bufs=2, space="PSUM"))
            g_pool = upctx.enter_context(
                tc.tile_pool(name="g_ps", bufs=2, space="PSUM"))
            upt = upctx.enter_context(tc.tile_pool(name="upt", bufs=2))
            for w in range(NW):
                oldw = upt.tile([64, 9, 128], fp32, tag="oldw")
                nc.sync.dma_start(
                    oldw[:], oldT_d[:, :].rearrange(
                        "p (q n) -> p q n", q=9)[:, :, w * 128:(w + 1) * 128])
                neww = upt.tile([64, 9, 128], fp32, tag="neww")
                y_ps = y_pool.tile([64, 9 * 128], fp32, tag="yps")
                for kg in range(9):
                    l = 0 if kg == 0 else (1 if kg <= 3 else 2)
                    nc.tensor.matmul(
                        y_ps[:, kg * 128:(kg + 1) * 128],
                        lin_sb[l][:],
                        agg_sb[:, w, kg % 2, kg // 2, :],
                        start=kg in (0, 4, 8), stop=True,
                        skip_group_check=True)
                y0g = upt.tile([C, 128], fp32, tag="y0g")
                nc.scalar.activation(y0g[:], y_ps[:, 0:128], AF.Sigmoid)
                y0s = upt.tile([C, 128], fp32, tag="y0s")
                nc.vector.tensor_tensor(out=y0s[:], in0=y_ps[:, 0:128],
                                        in1=y0g[:], op=ALU.mult)
                nc.vector.tensor_tensor(out=neww[:, 0, :], in0=y0s[:],
                                        in1=oldw[:, 0, :], op=ALU.add)
                g_ps = g_pool.tile([C, 2, 128], fp32, tag="gps")
                for l in (1, 2):
                    nc.tensor.matmul(g_ps[:, l - 1, :], gw_sb[l - 1][:],
                                     neww[:, 0, :], start=(l == 1), stop=True,
                                     skip_group_check=True)
                gts = upt.tile([C, 2, 128], fp32, tag="gts")
                for l in (1, 2):
                    nc.scalar.activation(gts[:, l - 1, :], g_ps[:, l - 1, :],
                                         AF.Sigmoid, bias=gb_sb[l - 1][:, 0:1])
                gy = upt.tile([C, 8, 128], fp32, tag="gy")
                nc.vector.tensor_tensor(
                    out=gy[:, 0:3, :],
                    in0=y_ps[:].rearrange("p (q n) -> p q n", q=9)[:, 1:4, :],
                    in1=gts[:, 0:1, :].broadcast_to((C, 3, 128)),
                    op=ALU.mult)
                nc.vector.tensor_tensor(
                    out=gy[:, 3:8, :],
                    in0=y_ps[:].rearrange("p (q n) -> p q n", q=9)[:, 4:9, :],
                    in1=gts[:, 1:2, :].broadcast_to((C, 5, 128)),
                    op=ALU.mult)
                nc.vector.tensor_tensor(out=neww[:, 1:9, :], in0=gy[:],
                                        in1=oldw[:, 1:9, :], op=ALU.add)
                nc.sync.dma_start(
                    newT_d[:, :].rearrange("p (q n) -> p q n",
                                           q=9)[:, :, w * 128:(w + 1) * 128],
                    neww[:])

    nc.compile()
    return nc


def _emit_tp_l2(nc, ALU, xg, w_sb, shq_t, msgs, pp, zjp, zgp):
    """P products, cg-folded zjQ scalings, FMA as adds (+8 ratio stts).

    All on DVE: single in-order queue, no cross-engine ping-pong; GpSimd
    has ~9us fixed cost per elementwise op and ScalarE cannot multiply
    two tensors, so the TP bulk lives here.
    """
    import concourse.mybir as mybir
    bf16 = mybir.dt.bfloat16
    fp32 = mybir.dt.float32

    GP_JS = ()          # GpSimd streams ~0.4 elem/cyc and zjG bufs=1
                        # serialized groups -> ~19us DVE stall per group;
                        # cheaper to keep all zjq on DVE (nnz-only now)
    EXPC = 8            # ping-pong: 2 x 8x8x32 fp32 = 16KB
    P0 = pp.tile([128, BP, SCHED2_L2[0]["nblk"] * C], bf16, name="P0", tag="P0")
    P12 = pp.tile([128, BP, SCHED2_L2[2]["nblk"] * C], bf16,
                  name="P12", tag="P12")
    zj12 = zjp.tile([128, BP, SCHED2_L2[2]["nblk"] * C], bf16,
                    name="zj12", tag="zj12")
    exps = [zgp.tile([128, BP, EXPC, C // 2], fp32, name=f"exp{i}",
                     tag=f"exp{i}") for i in range(2)]
    ppg = [0]

    def emit_zjq(eng, dst, P, qoff, segs, fma_runs=None):
        # shq host-packed as fp32 = (v,v) bf16 pair: ScalarE broadcast-
        # expands at half the element count, DVE bitcasts back to bf16 and
        # multiplies with unit-stride in1 -> 2x_1P mode. Chunks avoid
        # splitting segments (fewer consume ops), and FMA runs are emitted
        # as soon as their z-range is consumed so DVE never stalls on the
        # expansion of a later chunk while FMA work is ready.
        chunks, cur, used = [], [], 0
        for (z0, L) in segs:
            if cur and used + L > EXPC and L <= EXPC:
                chunks.append(cur)
                cur, used = [], 0
            while L > 0:
                take = min(L, EXPC - used)
                cur.append((z0, take, used))
                z0, L, used = z0 + take, L - take, used + take
                if used == EXPC:
                    chunks.append(cur)
                    cur, used = [], 0
        if cur:
            chunks.append(cur)
        rem_add, rem_stt = ([], []) if fma_runs is None else fma_runs
        rem_add, rem_stt = list(rem_add), list(rem_stt)
        consumed = set()
        for ch in chunks:
            exp = exps[ppg[0]]
            ppg[0] = (ppg[0] + 1) % 2
            for (z0, L, eo) in ch:
                nc.scalar.copy(
                    exp[:, :, eo:eo + L, :],
                    shq_t[:, :, qoff + z0:qoff + z0 + L, None].broadcast_to(
                        (128, BP, L, C // 2)))
            expb = exp[:].bitcast(mybir.dt.bfloat16)
            for (z0, L, eo) in ch:
                eng.tensor_tensor(
                    out=dst[:].rearrange("p b (n c) -> p b n c",
                                         c=C)[:, :, z0:z0 + L, :],
                    in0=P[:].rearrange("p b (n c) -> p b n c",
                                       c=C)[:, :, z0:z0 + L, :],
                    in1=expb[:, :, eo:eo + L, :],
                    op=ALU.mult)
                consumed.update(range(z0, z0 + L))
            for (jj, z0, k0, L) in [r for r in rem_add if all(
                    z in consumed for z in range(r[1], r[1] + r[3]))]:
                rem_add.remove((jj, z0, k0, L))
                nc.vector.tensor_tensor(
                    out=msgs[:, :, k0 * C:(k0 + L) * C],
                    in0=dst[:, :, z0 * C:(z0 + L) * C],
                    in1=msgs[:, :, k0 * C:(k0 + L) * C],
                    op=ALU.add)
            for (jj, z, k, ratio) in [r for r in rem_stt
                                      if r[1] in consumed]:
                rem_stt.remove((jj, z, k, ratio))
                nc.vector.scalar_tensor_tensor(
                    out=msgs[:, :, k * C:(k + 1) * C],
                    in0=dst[:, :, z * C:(z + 1) * C],
                    scalar=float(ratio),
                    in1=msgs[:, :, k * C:(k + 1) * C],
                    op0=ALU.mult, op1=ALU.add)
        assert not rem_add and not rem_stt, (rem_add, rem_stt)

    def emit_fma(sch, j, zj):
        for (jj, z0, k0, L) in sch["runs_add"]:
            if jj != j:
                continue
            nc.vector.tensor_tensor(
                out=msgs[:, :, k0 * C:(k0 + L) * C],
                in0=zj[:, :, z0 * C:(z0 + L) * C],
                in1=msgs[:, :, k0 * C:(k0 + L) * C],
                op=ALU.add)
        for (jj, z, k, ratio) in sch["runs_stt"]:
            if jj != j:
                continue
            nc.vector.scalar_tensor_tensor(
                out=msgs[:, :, k * C:(k + 1) * C],
                in0=zj[:, :, z * C:(z + 1) * C],
                scalar=float(ratio),
                in1=msgs[:, :, k * C:(k + 1) * C],
                op0=ALU.mult, op1=ALU.add)

    for gi, sch in enumerate(SCHED2_L2):
        nblk = sch["nblk"]
        P = P0 if gi == 0 else P12
        for (p, ig0, ni, z0) in sch["pops"]:
            nc.vector.tensor_tensor(
                out=P[:].rearrange("p b (n c) -> p b n c",
                                   c=C)[:, :, z0:z0 + ni, :],
                in0=xg[:].rearrange("p b (n c) -> p b n c",
                                    c=C)[:, :, ig0:ig0 + ni, :],
                in1=w_sb[:, :, None, p * C:(p + 1) * C].broadcast_to(
                    (128, BP, ni, C)),
                op=ALU.mult)
        if gi > 0:
            # kick GpSimd's share first so it overlaps DVE's other j's
            for (j, qoff, segs) in sch["zjq"]:
                if j in GP_JS:
                    emit_zjq(nc.gpsimd, zjG[j], P, qoff, segs)
        for (j, qoff, segs) in sch["zjq"]:
            if gi > 0 and j in GP_JS:
                continue
            dst = msgs if gi == 0 else zj12
            runs = None if gi == 0 else (
                [r for r in sch["runs_add"] if r[0] == j],
                [r for r in sch["runs_stt"] if r[0] == j])
            emit_zjq(nc.vector, dst, P, qoff, segs, runs)
        if gi > 0:
            for j in GP_JS:
                if any(jj == j for (jj, _, _) in sch["zjq"]):
                    emit_fma(sch, j, zjG[j])


def _emit_tp_l1(nc, ALU, xg, w_sb, shB, msgs, pp):
    """msgs[k] = x * w'_k * sh_k ; w' host-expanded+cg-folded to 9 kg blocks."""
    import concourse.mybir as mybir
    bf16 = mybir.dt.bfloat16
    tmp = pp.tile([128, BP, 9, C], bf16, name="l1tmp", tag="l1tmp")
    nc.vector.tensor_tensor(
        out=tmp[:],
        in0=w_sb[:].rearrange("p b (n c) -> p b n c", c=C),
        in1=shB[:].bitcast(bf16),
        op=ALU.mult)
    nc.vector.tensor_tensor(
        out=msgs[:].rearrange("p b (n c) -> p b n c", c=C),
        in0=tmp[:],
        in1=xg[:, :, None, 0:C].broadcast_to((128, BP, 9, C)),
        op=ALU.mult)


# ---------------- host orchestration ----------------
def _chunked_T(feats_own):
    """[NPC, 576] -> kg-blocked transposed [64, 9*NPC]."""
    out = np.empty((64, 9 * NPC), np.float32)
    for kg in range(9):
        out[:, kg * NPC:(kg + 1) * NPC] = feats_own[:, kg * 64:(kg + 1) * 64].T
    return out


def _unchunk_T(newT):
    """[64, 9*NPC] -> [NPC, 576]."""
    out = np.empty((NPC, 576), np.float32)
    for kg in range(9):
        out[:, kg * 64:(kg + 1) * 64] = newT[:, kg * NPC:(kg + 1) * NPC].T
    return out


_CACHE = {}


def _prep(positions, senders, receivers):
    key = (senders.tobytes(), receivers.tobytes(), positions.tobytes())
    if _CACHE.get("key") == key:
        return _CACHE["val"]
    sh_eff, basis = edge_geometry(positions, senders, receivers)
    owner, local, nodes_of, _ = partition_graph(receivers)
    deg_bin = np.zeros(NCORES * NW, np.int64)
    np.add.at(deg_bin, owner[receivers] * NW + local[receivers] // 128, 1)
    tpw = (int(deg_bin.max()) + 127) // 128
    T = NW * tpw
    assert T % BP == 0
    perm = build_core_edges(receivers, owner, local, tpw)

    valid = perm >= 0
    eg = np.where(valid, perm, 0)
    snd = np.where(valid, senders[eg], 0).astype(np.int16)      # [NC, T*128]
    shp_e = sh_eff[eg] * valid[..., None]                        # [NC, T*128, 9]
    bas_e = basis[eg] * valid[..., None]                         # [NC, T*128, 8]
    lr = np.where(valid, local[receivers[eg]], 0)

    NG = T // BP
    inv = np.float32(1.0 / np.sqrt(AVG_NN))
    sidx = np.empty((NCORES, 128, T * 128 // 16), np.int16)
    shp_h = np.empty((NCORES, 128, T, 9), np.float32)
    shq_h = np.empty((NCORES, 128, T, NSHQ), np.float32)
    jcols = np.array([j for (j, cg) in SHQ_COLS])
    cgv = np.array([cg for (j, cg) in SHQ_COLS], np.float32)
    bas_h = np.empty((NCORES, NG, 8, BP * 128), BF16)
    smat_h = np.zeros((NCORES, 128, T, 128), BF16)
    for k in range(NCORES):
        s = snd[k].reshape(T * 8, 16)
        sidx[k] = np.tile(s.T, (8, 1))
        shp_f = shp_e[k].reshape(T, 128, 9).transpose(1, 0, 2)
        # fp32-packed (v, v) bf16 pair so on-chip broadcast-expansion moves
        # half the elements (bitcast back to bf16 at the consumer)
        up = shp_f.astype(BF16).view(np.uint16).astype(np.uint32)
        shp_h[k] = ((up << 16) | up).view(np.float32)
        u = (shp_f[:, :, jcols] * cgv[None, None, :]).astype(
            BF16).view(np.uint16).astype(np.uint32)
        shq_h[k] = ((u << 16) | u).view(np.float32)
        bas_h[k] = bas_e[k].reshape(NG, BP * 128, 8).transpose(0, 2, 1).astype(BF16)
        v = valid[k]
        e_slots = np.arange(T * 128)
        p_, t_ = e_slots % 128, e_slots // 128
        cols = lr[k] - (t_ // tpw) * 128
        ok = v & (cols >= 0) & (cols < 128)
        sm = np.zeros((128, T, 128), np.float32)
        sm[p_[ok], t_[ok], cols[ok]] = inv
        smat_h[k] = sm.astype(BF16)
    val = dict(T=T, NG=NG, tpw=tpw, nodes_of=nodes_of, sidx=sidx,
               shp_h=shp_h, bas_h=bas_h, smat_h=smat_h, shq_h=shq_h)
    _CACHE["key"], _CACHE["val"] = key, val
    return val


EXEC_NS = []


def _run_layer(nc, pre, ftab, oldT_by_core, lw):
    import os
    from concourse.bass_utils import run_bass_kernel_spmd
    in_maps = []
    for k in range(NCORES):
        m = dict(ftab=ftab,
                 sidx=pre["sidx"][k],
                 shp=pre["shp_h"][k],
                 shq=pre["shq_h"][k],
                 basisT=pre["bas_h"][k],
                 smat=pre["smat_h"][k],
                 oldT=oldT_by_core[k],
                 w1=lw["w1"], b1=lw["b1"], w2=lw["w2"], b2=lw["b2"],
                 w3=lw["w3"], lin0=lw["lin"][0], lin1=lw["lin"][1],
                 lin2=lw["lin"][2], gw0=lw["gw"][0], gw1=lw["gw"][1],
                 gb0=lw["gb"][0], gb1=lw["gb"][1])
        in_maps.append(m)
    trace = bool(os.environ.get("KERNEL_TRACE"))
    res = run_bass_kernel_spmd(nc, in_maps, list(range(NCORES)), trace=trace,
                               trace_cores=list(range(NCORES)) if trace else None)
    if trace and res.exec_time_ns is not None:
        EXEC_NS.append(res.exec_time_ns)
    return [res.results[k]["newT"] for k in range(NCORES)]


def _layer_weights(inputs, i, layer2):
    f32 = np.float32
    w3 = np.ascontiguousarray(inputs["mlp_w3"][i], f32)
    if layer2:
        w3p = w3.astype(BF16)
    else:
        w3p = np.empty((H, 576), BF16)
        for kg in range(9):
            p = L1_PATH_OF_K[kg]
            w3p[:, kg * C:(kg + 1) * C] = (
                w3[:, p * C:(p + 1) * C] * L1_CG_OF_K[kg]).astype(BF16)
    return dict(
        w1=np.ascontiguousarray(inputs["mlp_w1"][i], f32).astype(BF16),
        b1=np.ascontiguousarray(inputs["mlp_b1"][i], f32).reshape(H, 1),
        w2=np.ascontiguousarray(inputs["mlp_w2"][i], f32).astype(BF16),
        b2=np.ascontiguousarray(inputs["mlp_b2"][i], f32).reshape(H, 1),
        w3=w3p,
        lin=[np.ascontiguousarray(inputs["lin_self"][i, l], f32)
             for l in range(3)],
        gw=[np.ascontiguousarray(inputs["gate_w"][i, l], f32)
            for l in range(2)],
        gb=[np.ascontiguousarray(inputs["gate_b"][i, l], f32).reshape(C, 1)
            for l in range(2)],
    )


_KERNEL_CACHE = {}


def _get_kernels(T):
    if T not in _KERNEL_CACHE:
        _KERNEL_CACHE[T] = (build_layer_kernel(False, T),
                            build_layer_kernel(True, T))
    return _KERNEL_CACHE[T]


def _pack_ftab(table, ncols):
    out = np.zeros((N_NODES, ncols), BF16)
    used = min(ncols, table.shape[1])
    out[:, :used] = table[:, :used].astype(BF16)
    return out


def kernel(**inputs):
    positions = np.asarray(inputs["positions"], np.float32)
    species = np.asarray(inputs["species"]).astype(np.int64)
    senders = np.asarray(inputs["senders"]).astype(np.int64)
    receivers = np.asarray(inputs["receivers"]).astype(np.int64)

    pre = _prep(positions, senders, receivers)
    T = pre["T"]
    nc1, nc2 = _get_kernels(T)
    nodes_of = pre["nodes_of"]

    # initial features: x0 from species embedding (host; tiny)
    x0 = (np.asarray(inputs["embed"], np.float32)[species]
          @ np.asarray(inputs["w_proj"], np.float32))          # [N, 64]
    table = np.zeros((N_NODES, F), np.float32)
    table[:, 0:C] = x0

    # ---- layer 1 ----
    oldT = [_chunked_T(table[nodes_of[k]]) for k in range(NCORES)]
    lw = _layer_weights(inputs, 0, False)
    newT = _run_layer(nc1, pre, _pack_ftab(table, GCOLP_L1), oldT, lw)

    table2 = np.empty((N_NODES, F), np.float32)
    for k in range(NCORES):
        table2[nodes_of[k]] = _unchunk_T(newT[k])

    # ---- layer 2 ----
    lw = _layer_weights(inputs, 1, True)
    newT2 = _run_layer(nc2, pre, _pack_ftab(table2, GCOLP_L2), newT, lw)

    table3 = np.empty((N_NODES, F), np.float32)
    for k in range(NCORES):
        table3[nodes_of[k]] = _unchunk_T(newT2[k])

    # ---- output: reorder component-major -> reference layout + alpha ----
    t3 = table3.reshape(N_NODES, 9, C)
    out = np.empty((N_NODES, F), np.float32)
    out[:, 0:64] = t3[:, 0]
    out[:, 64:256] = (0.5 * t3[:, 1:4]).transpose(0, 2, 1).reshape(N_NODES, 192)
    out[:, 256:576] = (0.25 * t3[:, 4:9]).transpose(0, 2, 1).reshape(N_NODES, 320)
    return out



# revision 27
# speedup vs baseline: 1.3984x; 1.0234x over previous
"""NequIP GNN message-passing kernel for 8 Trainium2 NeuronCores — v2.

Receiver-sharded graph parallelism (per sharding hint): host LPT-assigns the
8192 nodes to 64 (core, window) bins of 128, each core owns 8 windows = 1024
nodes + their in-edges, sorted by window, padded to 128-edge tiles.

v2 device pipeline (vs v1): bf16 edge pipeline end-to-end with fp32 PSUM
accumulation; per-edge-scalar stages batched across 8-tile groups via
stride-0 broadcast access patterns; P-products collapsed per-path (w operand
broadcast over l1 components); CG-FMA stage merged into diagonal runs with
immediate scalars; segment-sum via paired-kg bf16 matmuls; radial MLP with
fused Silu activations; elementwise work split across Vector/GpSimd/Scalar.
"""
import math
import numpy as np
import ml_dtypes

BF16 = ml_dtypes.bfloat16

# ---------------- model constants ----------------
N_NODES, N_EDGES = 8192, 131072
C, H, NRAD = 64, 64, 8
R_MAX, AVG_NN = 5.0, 16.0
NCORES, NPC = 8, 1024
NW = NPC // 128
F = 9 * C
LS = (0, 1, 2)
PATHS = [(l1, l2, l3) for l1 in LS for l2 in LS for l3 in LS
         if abs(l1 - l2) <= l3 <= l1 + l2]
LOFF = {0: 0, 1: 1, 2: 4}
J_OF_L2 = {0: [0], 1: [1, 2, 3], 2: [4, 5, 6, 7, 8]}
BP = 8                      # tiles per group (batched in op free dims)
GCOLP_L2 = 640              # padded gather row (bf16): 1280B % 256 == 0
GCOLP_L1 = 128              # 256B % 256 == 0


# ---------------- real Clebsch-Gordan coefficients ----------------
def _cg_scalar(j1, m1, j2, m2, j3, m3):
    f = math.factorial
    if m1 + m2 != m3:
        return 0.0
    pre = ((2*j3+1) * f(j1+j2-j3) * f(j1-j2+j3) * f(-j1+j2+j3)
           / f(j1+j2+j3+1)) ** 0.5
    pre *= (f(j1+m1)*f(j1-m1)*f(j2+m2)*f(j2-m2)*f(j3+m3)*f(j3-m3)) ** 0.5
    s = 0.0
    for k in range(max(0, j2-j3-m1, j1+m2-j3), min(j1+j2-j3, j1-m1, j2+m2)+1):
        s += (-1)**k / (f(k)*f(j1+j2-j3-k)*f(j1-m1-k)
                        * f(j2+m2-k)*f(j3-j2+m1+k)*f(j3-j1-m2+k))
    return pre * s


def _U_real(l):
    U = np.zeros((2*l+1, 2*l+1), dtype=complex)
    s2 = 2 ** -0.5
    for m in range(-l, l+1):
        if m > 0:
            U[m+l, m+l] = (-1)**m * s2
            U[m+l, -m+l] = s2
        elif m == 0:
            U[l, l] = 1.0
        else:
            U[m+l, m+l] = 1j*s2
            U[m+l, -m+l] = -1j*(-1)**(-m)*s2
    return U


def _real_cg(l1, l2, l3):
    Cc = np.zeros((2*l1+1, 2*l2+1, 2*l3+1))
    for i1, m1 in enumerate(range(-l1, l1+1)):
        for i2, m2 in enumerate(range(-l2, l2+1)):
            m3 = m1 + m2
            if abs(m3) <= l3:
                Cc[i1, i2, m3+l3] = _cg_scalar(l1, m1, l2, m2, l3, m3)
    U1, U2, U3 = _U_real(l1), _U_real(l2), _U_real(l3)
    W = np.einsum('ia,jb,kc,abc->ijk', U1.conj(), U2.conj(), U3,
                  Cc.astype(complex))
    W = W.real if np.linalg.norm(W.real) >= np.linalg.norm(W.imag) else W.imag
    W = W / np.linalg.norm(W) * (2*l3+1) ** 0.5
    return np.asarray(W, dtype=np.float64)


CGS = [_real_cg(*p) for p in PATHS]


def build_schedule_l2():
    """Static TP structure for layer 2, grouped by l2.

    Per group: nblk, pops=[(path, ig0, ni, z0)], jlist, runs=[(j,z0,k0,L,cg)].
    """
    scheds = []
    for l2 in (0, 1, 2):
        ps = [p for p in range(len(PATHS)) if PATHS[p][1] == l2]
        blocks, block_of, pops = [], {}, []
        for p in ps:
            l1 = PATHS[p][0]
            ni = 2 * l1 + 1
            pops.append((p, LOFF[l1], ni, len(blocks)))
            for i in range(ni):
                block_of[(p, i)] = len(blocks)
                blocks.append((p, i))
        tset = set()
        for p in ps:
            l1, _, l3 = PATHS[p]
            cg = CGS[p]
            for i in range(2*l1+1):
                for j in range(2*l2+1):
                    for k in range(2*l3+1):
                        v = cg[i, j, k]
                        if abs(v) > 1e-12:
                            tset.add((LOFF[l2]+j, block_of[(p, i)],
                                      LOFF[l3]+k, round(float(v), 9)))
        runs, consumed = [], set()
        for t in sorted(tset):
            if t in consumed:
                continue
            j, z, k, cgv = t
            L = 0
            while (j, z+L, k+L, cgv) in tset and (j, z+L, k+L, cgv) not in consumed:
                consumed.add((j, z+L, k+L, cgv))
                L += 1
            runs.append((j, z, k, L, cgv))
        scheds.append(dict(l2=l2, nblk=len(blocks), pops=pops,
                           jlist=J_OF_L2[l2], runs=runs))
    return scheds


SCHED_L2 = build_schedule_l2()


def build_schedule2():
    """cg folded into per-(j,z) host scalars; FMA mostly tensor adds.

    Per group: nblk, pops, zjq=[(j, qoff)], runs_add=[(j,z0,k0,L)],
    runs_stt=[(j,z,k,ratio)]. qoff = column offset of (j,*) block in the
    concatenated shq table [128, T, 179]. cgfold[qoff+z] = cg of the primary
    (smallest-k) triple of (j,z); 0 for unused (z,j).
    """
    scheds, qoff, cgcols = [], 0, []
    for gi, sch in enumerate(SCHED_L2):
        nblk = sch["nblk"]
        tset = set()
        for (j, z, k, L, cg) in sch["runs"]:
            for i in range(L):
                tset.add((j, z + i, k + i, cg))
        per_jz = {}
        for (j, z, k, cg) in tset:
            per_jz.setdefault((j, z), []).append((k, cg))
        primary, runs_stt = set(), []
        cgf = {}
        for (j, z), ks in per_jz.items():
            ks.sort()
            k0, cg0 = ks[0]
            primary.add((j, z, k0))
            cgf[(j, z)] = cg0
            for (k1, cg1) in ks[1:]:
                runs_stt.append((j, z, k1, cg1 / cg0))
        runs_add, consumed = [], set()
        for t in sorted(primary):
            if t in consumed:
                continue
            j, z, k = t
            L = 0
            while (j, z + L, k + L) in primary and (j, z + L, k + L) not in consumed:
                consumed.add((j, z + L, k + L))
                L += 1
            runs_add.append((j, z, k, L))
        zjq = []
        for j in sch["jlist"]:
            # contiguous segments of z columns actually used by (j, z):
            # skips the ~28% of dense columns with cg == 0
            used = sorted(z for (jj, z) in cgf if jj == j)
            segs, s0, prev = [], None, None
            for z in used:
                if s0 is None:
                    s0 = prev = z
                elif z == prev + 1:
                    prev = z
                else:
                    segs.append((s0, prev - s0 + 1))
                    s0 = prev = z
            if s0 is not None:
                segs.append((s0, prev - s0 + 1))
            zjq.append((j, qoff, segs))
            for z in range(nblk):
                cgcols.append((j, cgf.get((j, z), 0.0)))
            qoff += nblk
        scheds.append(dict(nblk=nblk, pops=sch["pops"], zjq=zjq,
                           runs_add=runs_add, runs_stt=runs_stt))
    return scheds, cgcols


SCHED2_L2, SHQ_COLS = build_schedule2()
NSHQ = len(SHQ_COLS)

# layer-1 per-k path and cg (x is scalar-only: paths (0,l,l), j == k)
L1_PATH_OF_K = [0] + [1]*3 + [2]*5
L1_CG_OF_K = []
for _k in range(9):
    _p = L1_PATH_OF_K[_k]
    _l = PATHS[_p][2]
    _m = _k - LOFF[_l]
    L1_CG_OF_K.append(float(CGS[_p][0, _m, _m]))


# ---------------- host-side graph preprocessing ----------------
def edge_geometry(positions, senders, receivers):
    rel = (positions[receivers] - positions[senders]) / R_MAX
    d = np.linalg.norm(rel, axis=-1)
    u = rel / np.maximum(d, 1e-6)[:, None]
    x, y, z = u[:, 0], u[:, 1], u[:, 2]
    sh = np.empty((len(d), 9), np.float32)
    sh[:, 0] = 1.0
    sh[:, 1] = np.sqrt(3.0) * y
    sh[:, 2] = np.sqrt(3.0) * z
    sh[:, 3] = np.sqrt(3.0) * x
    sh[:, 4] = np.sqrt(15.0) * x * y
    sh[:, 5] = np.sqrt(15.0) * y * z
    sh[:, 6] = np.sqrt(5.0) / 2 * (3 * z * z - 1.0)
    sh[:, 7] = np.sqrt(15.0) * x * z
    sh[:, 8] = np.sqrt(15.0) / 2 * (x * x - y * y)
    freqs = np.arange(1, NRAD + 1, dtype=np.float64)
    xr = np.clip(d, 1e-4, 1.0)[:, None].astype(np.float64)
    basis = (np.sqrt(2.0) * np.sin(freqs * np.pi * xr) / xr).astype(np.float32)
    cut = (0.5 * (np.cos(np.pi * np.clip(d, 0.0, 1.0)) + 1.0)).astype(np.float32)
    return (sh * cut[:, None]).astype(np.float32), basis


def partition_graph(receivers):
    import heapq
    deg = np.bincount(receivers, minlength=N_NODES)
    order = np.argsort(-deg, kind="stable")
    nbins = NCORES * NW
    load = np.zeros(nbins, np.int64)
    cnt = np.zeros(nbins, np.int64)
    owner = np.empty(N_NODES, np.int32)
    local = np.empty(N_NODES, np.int32)
    heap = [(0, b) for b in range(nbins)]
    heapq.heapify(heap)
    for n in order:
        while True:
            l, b = heapq.heappop(heap)
            if cnt[b] < 128:
                break
        owner[n] = b // NW
        local[n] = (b % NW) * 128 + cnt[b]
        cnt[b] += 1
        load[b] += deg[n]
        if cnt[b] < 128:
            heapq.heappush(heap, (int(load[b]), b))
    nodes_of = np.empty((NCORES, NPC), np.int64)
    for n in range(N_NODES):
        nodes_of[owner[n], local[n]] = n
    return owner, local, nodes_of, int(load.max())


def build_core_edges(receivers, owner, local, tpw):
    T = NW * tpw
    perm = np.full((NCORES, T * 128), -1, np.int64)
    for k in range(NCORES):
        eids = np.where(owner[receivers] == k)[0]
        lr = local[receivers[eids]]
        o = np.argsort(lr, kind="stable")
        eids, lr = eids[o], lr[o]
        w_of = lr // 128
        for w in range(NW):
            sel = eids[w_of == w]
            assert len(sel) <= tpw * 128, "tiles-per-window overflow"
            base = w * tpw * 128
            perm[k, base:base + len(sel)] = sel
    return perm


# ---------------- bass kernel builder ----------------
def build_layer_kernel(layer2, T, debug=False):
    import concourse.bass as bass
    import concourse.bacc as bacc
    import concourse.tile as tile
    import concourse.mybir as mybir
    from contextlib import ExitStack

    fp32 = mybir.dt.float32
    bf16 = mybir.dt.bfloat16
    AF = mybir.ActivationFunctionType
    ALU = mybir.AluOpType

    NPATH = 15 if layer2 else 3
    GCOLP = GCOLP_L2 if layer2 else GCOLP_L1
    W3COL = NPATH * C if layer2 else 576   # L1 w3 host-expanded to 9 kg blocks
    E_PAD = T * 128
    NG = T // BP
    assert T % BP == 0 and T % NW == 0
    tpw = T // NW

    import os as _os
    STAGE = int(_os.environ.get("KV2_STAGE", "5"))
    nc = bacc.Bacc("TRN2", target_bir_lowering=False)

    ftab = nc.dram_tensor("ftab", [N_NODES, GCOLP], bf16, kind="ExternalInput")
    sidx = nc.dram_tensor("sidx", [128, E_PAD // 16], mybir.dt.int16,
                          kind="ExternalInput")
    shp_d = nc.dram_tensor("shp", [128, T, 9], fp32, kind="ExternalInput")
    shq_d = nc.dram_tensor("shq", [128, T, NSHQ], fp32, kind="ExternalInput")
    basT_d = nc.dram_tensor("basisT", [NG, 8, BP * 128], bf16,
                            kind="ExternalInput")
    smat_d = nc.dram_tensor("smat", [128, T, 128], bf16, kind="ExternalInput")
    oldT_d = nc.dram_tensor("oldT", [64, 9 * NPC], fp32, kind="ExternalInput")
    w1_d = nc.dram_tensor("w1", [8, H], bf16, kind="ExternalInput")
    b1_d = nc.dram_tensor("b1", [H, 1], fp32, kind="ExternalInput")
    w2_d = nc.dram_tensor("w2", [H, H], bf16, kind="ExternalInput")
    b2_d = nc.dram_tensor("b2", [H, 1], fp32, kind="ExternalInput")
    w3_d = nc.dram_tensor("w3", [H, W3COL], bf16, kind="ExternalInput")
    lin_d = [nc.dram_tensor(f"lin{l}", [C, C], fp32, kind="ExternalInput")
             for l in range(3)]
    gw_d = [nc.dram_tensor(f"gw{l}", [C, C], fp32, kind="ExternalInput")
            for l in range(2)]
    gb_d = [nc.dram_tensor(f"gb{l}", [C, 1], fp32, kind="ExternalInput")
            for l in range(2)]
    newT_d = nc.dram_tensor("newT", [64, 9 * NPC], fp32,
                            kind="ExternalOutput")
    if debug:
        dbg_xg = nc.dram_tensor("dbg_xg", [128, BP, GCOLP], fp32,
                                kind="ExternalOutput")
        dbg_w = nc.dram_tensor("dbg_w", [128, BP, W3COL], fp32,
                               kind="ExternalOutput")
        dbg_msgs = nc.dram_tensor("dbg_msgs", [128, BP, F], fp32,
                                  kind="ExternalOutput")
        dbg_agg = nc.dram_tensor("dbg_agg", [128, NW, 640], fp32,
                                 kind="ExternalOutput")

    with tile.TileContext(nc) as tc, ExitStack() as ctx:
        consts = ctx.enter_context(tc.tile_pool(name="consts", bufs=1))
        idx_sb = consts.tile([128, E_PAD // 16], mybir.dt.int16)
        nc.sync.dma_start(idx_sb[:], sidx[:])
        w1_sb = consts.tile([8, H], bf16)
        nc.sync.dma_start(w1_sb[:], w1_d[:])
        b1_sb = consts.tile([H, 1], fp32)
        nc.sync.dma_start(b1_sb[:], b1_d[:])
        w2_sb = consts.tile([H, H], bf16)
        nc.sync.dma_start(w2_sb[:], w2_d[:])
        b2_sb = consts.tile([H, 1], fp32)
        nc.sync.dma_start(b2_sb[:], b2_d[:])
        w3_sb = consts.tile([H, W3COL], bf16)
        nc.sync.dma_start(w3_sb[:], w3_d[:])
        lin_sb = [consts.tile([C, C], fp32, name=f"lin{l}", tag=f"lin{l}")
                  for l in range(3)]
        for l in range(3):
            nc.sync.dma_start(lin_sb[l][:], lin_d[l][:])
        gw_sb = [consts.tile([C, C], fp32, name=f"gw{l}", tag=f"gw{l}")
                 for l in range(2)]
        gb_sb = [consts.tile([C, 1], fp32, name=f"gb{l}", tag=f"gb{l}")
                 for l in range(2)]
        for l in range(2):
            nc.sync.dma_start(gw_sb[l][:], gw_d[l][:])
            nc.sync.dma_start(gb_sb[l][:], gb_d[l][:])
        agg_sb = consts.tile([64, NW, 2, 5, 128], fp32)

        with ExitStack() as psctx:
            iop = psctx.enter_context(
                tc.tile_pool(name="iop", bufs=2 if layer2 else 3))
            aggt = psctx.enter_context(tc.tile_pool(name="aggt", bufs=1))
            wp = psctx.enter_context(tc.tile_pool(name="wp", bufs=1))
            msgp = psctx.enter_context(tc.tile_pool(name="msgp", bufs=2))
            shbp = psctx.enter_context(tc.tile_pool(name="shbp", bufs=1))
            pp = psctx.enter_context(tc.tile_pool(name="pp", bufs=1))
            zjp = psctx.enter_context(tc.tile_pool(name="zjp", bufs=1))
            zgp = psctx.enter_context(tc.tile_pool(name="zgp", bufs=1))
            h_ps = psctx.enter_context(
                tc.tile_pool(name="h_ps", bufs=2, space="PSUM"))
            w_ps_pool = psctx.enter_context(
                tc.tile_pool(name="w_ps", bufs=2, space="PSUM"))
            agg_pool = psctx.enter_context(
                tc.tile_pool(name="agg_ps", bufs=2, space="PSUM"))

            agg_open = {}

            for g in range(NG):
                t0 = g * BP
                xg = iop.tile([128, BP, GCOLP], bf16, tag="xg")
                nc.gpsimd.dma_gather(
                    out_ap=xg[:],
                    in_ap=ftab[:],
                    idxs_ap=idx_sb[:, g * (BP * 8):(g + 1) * (BP * 8)],
                    num_idxs=BP * 128,
                    num_idxs_reg=BP * 128,
                    elem_size=GCOLP,
                )
                if not layer2:
                    shp_t = iop.tile([128, BP, 9], fp32, tag="shp")
                    nc.sync.dma_start(shp_t[:], shp_d[:, t0:t0 + BP, :])
                if layer2:
                    shq_t = iop.tile([128, BP, NSHQ], fp32, tag="shq")
                    nc.sync.dma_start(shq_t[:], shq_d[:, t0:t0 + BP, :])
                smt = iop.tile([128, BP, 128], bf16, tag="smt")
                nc.sync.dma_start(smt[:], smat_d[:, t0:t0 + BP, :])
                bas = iop.tile([8, BP * 128], bf16, tag="bas")
                nc.sync.dma_start(bas[:], basT_d[g, :, :])

                if STAGE < 2:
                    continue
                if not layer2:
                    # sh broadcast table for L1: fp32 = packed (v,v) bf16
                    # pair, so ScalarE moves half the elements (1x mode)
                    shB = shbp.tile([128, BP, 9, C // 2], fp32, tag="shB")
                    nc.scalar.copy(
                        shB[:],
                        shp_t[:, :, :, None].broadcast_to(
                            (128, BP, 9, C // 2)))

                # ---- radial MLP (transposed; fused Silu) ----
                h1s = iop.tile([H, BP * 128], bf16, tag="h1s")
                h2s = iop.tile([H, BP * 128], bf16, tag="h2s")
                sg = iop.tile([H, BP * 128], bf16, tag="sg")
                for c0 in range(0, BP * 128, 512):
                    h1p = h_ps.tile([H, 512], fp32, tag="h")
                    nc.tensor.matmul(h1p[:], w1_sb[:],
                                     bas[:, c0:c0 + 512], start=True, stop=True)
                    nc.scalar.activation(sg[:, c0:c0 + 512], h1p[:],
                                         AF.Sigmoid, bias=b1_sb[:, 0:1])
                    nc.vector.scalar_tensor_tensor(
                        out=h1s[:, c0:c0 + 512], in0=h1p[:],
                        scalar=b1_sb[:, 0:1], in1=sg[:, c0:c0 + 512],
                        op0=ALU.add, op1=ALU.mult)
                for c0 in range(0, BP * 128, 512):
                    h2p = h_ps.tile([H, 512], fp32, tag="h")
                    nc.tensor.matmul(h2p[:], w2_sb[:],
                                     h1s[:, c0:c0 + 512], start=True, stop=True)
                    nc.scalar.activation(sg[:, c0:c0 + 512], h2p[:],
                                         AF.Sigmoid, bias=b2_sb[:, 0:1])
                    nc.vector.scalar_tensor_tensor(
                        out=h2s[:, c0:c0 + 512], in0=h2p[:],
                        scalar=b2_sb[:, 0:1], in1=sg[:, c0:c0 + 512],
                        op0=ALU.add, op1=ALU.mult)

                # ---- per-tile edge weights w = h2s_t.T @ W3 (bf16 out) ----
                # one 1-bank PSUM tile per 512-col chunk, double-buffered:
                # tile t+1's matmul overlaps tile t's PSUM->SBUF copy
                w_sb = wp.tile([128, BP, W3COL], bf16, tag="wsb")
                for bt in range(BP):
                    for c0 in range(0, W3COL, 512):
                        c1 = min(c0 + 512, W3COL)
                        w_ps = w_ps_pool.tile([128, 512], fp32, tag="wps")
                        nc.tensor.matmul(w_ps[:, 0:c1 - c0],
                                         h2s[:, bt * 128:(bt + 1) * 128],
                                         w3_sb[:, c0:c1], start=True, stop=True)
                        # L1 is ScalarE-gated: split PSUM->SBUF copies with
                        # DVE (which idles ~65% there); L2 keeps ScalarE
                        # (DVE is the L2 bottleneck)
                        if layer2 or bt % 2 == 0:
                            nc.scalar.copy(w_sb[:, bt, c0:c1],
                                           w_ps[:, 0:c1 - c0])
                        else:
                            nc.vector.tensor_copy(out=w_sb[:, bt, c0:c1],
                                                  in_=w_ps[:, 0:c1 - c0])

                if STAGE < 3:
                    continue
                # ---- tensor product ----
                msgs = msgp.tile([128, BP, F], bf16, tag="msgs")
                if layer2:
                    _emit_tp_l2(nc, ALU, xg, w_sb, shq_t, msgs, pp, zjp, zgp)
                else:
                    _emit_tp_l1(nc, ALU, xg, w_sb, shB, msgs, pp)

                if debug and g == 0:
                    dxg = pp.tile([128, BP, GCOLP], fp32, tag="dxg")
                    nc.vector.tensor_copy(out=dxg[:], in_=xg[:])
                    nc.sync.dma_start(dbg_xg[:], dxg[:])
                    dw = pp.tile([128, BP, W3COL], fp32, tag="dw")
                    nc.vector.tensor_copy(out=dw[:], in_=w_sb[:])
                    nc.sync.dma_start(dbg_w[:], dw[:])
                    dmg = pp.tile([128, BP, F], fp32, tag="dmg")
                    nc.vector.tensor_copy(out=dmg[:], in_=msgs[:])
                    nc.sync.dma_start(dbg_msgs[:], dmg[:])

                if STAGE < 4:
                    continue
                # ---- segment sum: paired-kg bf16 matmuls, PSUM-accumulated ----
                for bt in range(BP):
                    t = t0 + bt
                    w, t_in_w = t // tpw, t % tpw
                    if w not in agg_open:
                        agg_open[w] = agg_pool.tile([128, 640], fp32,
                                                    name="aggps", tag="aggps")
                    ps = agg_open[w]
                    first, last = t_in_w == 0, t_in_w == tpw - 1
                    for pair in range(5):
                        c0 = pair * 128
                        cw = 128 if pair < 4 else 64
                        nc.tensor.matmul(
                            ps[0:cw, c0:c0 + 128],
                            msgs[:, bt, c0:c0 + cw],
                            smt[:, bt, :],
                            start=first and pair in (0, 4),
                            stop=last, skip_group_check=True)
                    if last:
                        aps = agg_open.pop(w)
                        afp = aggt.tile([128, 640], fp32, tag="afp")
                        nc.scalar.copy(afp[:, 0:512], aps[:, 0:512])
                        nc.scalar.copy(afp[0:64, 512:640], aps[0:64, 512:640])
                        nc.sync.dma_start(
                            agg_sb[:, w, 0, :, :],
                            afp[0:64, :].rearrange("p (q n) -> p q n", q=5))
                        nc.sync.dma_start(
                            agg_sb[:, w, 1, 0:4, :],
                            afp[64:128, 0:512].rearrange("p (q n) -> p q n",
                                                         q=4))



        # ---------------- per-window node update ----------------
        if STAGE < 5:
            with ExitStack() as upctx:
                upt = upctx.enter_context(tc.tile_pool(name="upt", bufs=2))
                for w in range(NW):
                    tmp = upt.tile([64, 9, 128], fp32, tag="pass")
                    nc.sync.dma_start(
                        tmp[:], oldT_d[:, :].rearrange(
                            "p (q n) -> p q n",
                            q=9)[:, :, w * 128:(w + 1) * 128])
                    nc.sync.dma_start(
                        newT_d[:, :].rearrange(
                            "p (q n) -> p q n",
                            q=9)[:, :, w * 128:(w + 1) * 128],
                        tmp[:])
            nc.compile()
            return nc
        with ExitStack() as upctx:
            y_pool = upctx.enter_context(
                tc.tile_pool(name="y_ps", bufs=2, space="PSUM"))
            g_pool = upctx.enter_context(
                tc.tile_pool(name="g_ps", bufs=2, space="PSUM"))
            upt = upctx.enter_context(tc.tile_pool(name="upt", bufs=2))
            for w in range(NW):
                oldw = upt.tile([64, 9, 128], fp32, tag="oldw")
                nc.sync.dma_start(
                    oldw[:], oldT_d[:, :].rearrange(
                        "p (q n) -> p q n", q=9)[:, :, w * 128:(w + 1) * 128])
                neww = upt.tile([64, 9, 128], fp32, tag="neww")
                y_ps = y_pool.tile([64, 9 * 128], fp32, tag="yps")
                for kg in range(9):
                    l = 0 if kg == 0 else (1 if kg <= 3 else 2)
                    nc.tensor.matmul(
                        y_ps[:, kg * 128:(kg + 1) * 128],
                        lin_sb[l][:],
                        agg_sb[:, w, kg % 2, kg // 2, :],
                        start=kg in (0, 4, 8), stop=True,
                        skip_group_check=True)
                y0g = upt.tile([C, 128], fp32, tag="y0g")
                nc.scalar.activation(y0g[:], y_ps[:, 0:128], AF.Sigmoid)
                y0s = upt.tile([C, 128], fp32, tag="y0s")
                nc.vector.tensor_tensor(out=y0s[:], in0=y_ps[:, 0:128],
                                        in1=y0g[:], op=ALU.mult)
                nc.vector.tensor_tensor(out=neww[:, 0, :], in0=y0s[:],
                                        in1=oldw[:, 0, :], op=ALU.add)
                g_ps = g_pool.tile([C, 2, 128], fp32, tag="gps")
                for l in (1, 2):
                    nc.tensor.matmul(g_ps[:, l - 1, :], gw_sb[l - 1][:],
                                     neww[:, 0, :], start=(l == 1), stop=True,
                                     skip_group_check=True)
                gts = upt.tile([C, 2, 128], fp32, tag="gts")
                for l in (1, 2):
                    nc.scalar.activation(gts[:, l - 1, :], g_ps[:, l - 1, :],
                                         AF.Sigmoid, bias=gb_sb[l - 1][:, 0:1])
                gy = upt.tile([C, 8, 128], fp32, tag="gy")
                nc.vector.tensor_tensor(
                    out=gy[:, 0:3, :],
                    in0=y_ps[:].rearrange("p (q n) -> p q n", q=9)[:, 1:4, :],
                    in1=gts[:, 0:1, :].broadcast_to((C, 3, 128)),
                    op=ALU.mult)
                nc.vector.tensor_tensor(
                    out=gy[:, 3:8, :],
                    in0=y_ps[:].rearrange("p (q n) -> p q n", q=9)[:, 4:9, :],
                    in1=gts[:, 1:2, :].broadcast_to((C, 5, 128)),
                    op=ALU.mult)
                nc.vector.tensor_tensor(out=neww[:, 1:9, :], in0=gy[:],
                                        in1=oldw[:, 1:9, :], op=ALU.add)
                nc.sync.dma_start(
                    newT_d[:, :].rearrange("p (q n) -> p q n",
                                           q=9)[:, :, w * 128:(w + 1) * 128],
                    neww[:])

    nc.compile()
    return nc


def _emit_tp_l2(nc, ALU, xg, w_sb, shq_t, msgs, pp, zjp, zgp):
    """P products, cg-folded zjQ scalings, FMA as adds (+8 ratio stts).

    All on DVE: single in-order queue, no cross-engine ping-pong; GpSimd
    has ~9us fixed cost per elementwise op and ScalarE cannot multiply
    two tensors, so the TP bulk lives here.
    """
    import concourse.mybir as mybir
    bf16 = mybir.dt.bfloat16
    fp32 = mybir.dt.float32

    GP_JS = ()          # GpSimd streams ~0.4 elem/cyc and zjG bufs=1
                        # serialized groups -> ~19us DVE stall per group;
                        # cheaper to keep all zjq on DVE (nnz-only now)
    EXPC = 8            # ping-pong: 2 x 8x8x32 fp32 = 16KB
    P0 = pp.tile([128, BP, SCHED2_L2[0]["nblk"] * C], bf16, name="P0", tag="P0")
    P12 = pp.tile([128, BP, SCHED2_L2[2]["nblk"] * C], bf16,
                  name="P12", tag="P12")
    zj12 = zjp.tile([128, BP, SCHED2_L2[2]["nblk"] * C], bf16,
                    name="zj12", tag="zj12")
    exps = [zgp.tile([128, BP, EXPC, C // 2], fp32, name=f"exp{i}",
                     tag=f"exp{i}") for i in range(2)]
    ppg = [0]

    def emit_zjq(eng, dst, P, qoff, segs, fma_runs=None):
        # shq host-packed as fp32 = (v,v) bf16 pair: ScalarE broadcast-
        # expands at half the element count, DVE bitcasts back to bf16 and
        # multiplies with unit-stride in1 -> 2x_1P mode. Chunks avoid
        # splitting segments (fewer consume ops), and FMA runs are emitted
        # as soon as their z-range is consumed so DVE never stalls on the
        # expansion of a later chunk while FMA work is ready.
        chunks, cur, used = [], [], 0
        for (z0, L) in segs:
            if cur and used + L > EXPC and L <= EXPC:
                chunks.append(cur)
                cur, used = [], 0
            while L > 0:
                take = min(L, EXPC - used)
                cur.append((z0, take, used))
                z0, L, used = z0 + take, L - take, used + take
                if used == EXPC:
                    chunks.append(cur)
                    cur, used = [], 0
        if cur:
            chunks.append(cur)
        rem_add, rem_stt = ([], []) if fma_runs is None else fma_runs
        rem_add, rem_stt = list(rem_add), list(rem_stt)
        consumed = set()
        for ch in chunks:
            exp = exps[ppg[0]]
            ppg[0] = (ppg[0] + 1) % 2
            for (z0, L, eo) in ch:
                nc.scalar.copy(
                    exp[:, :, eo:eo + L, :],
                    shq_t[:, :, qoff + z0:qoff + z0 + L, None].broadcast_to(
                        (128, BP, L, C // 2)))
            expb = exp[:].bitcast(mybir.dt.bfloat16)
            for (z0, L, eo) in ch:
                eng.tensor_tensor(
                    out=dst[:].rearrange("p b (n c) -> p b n c",
                                         c=C)[:, :, z0:z0 + L, :],
                    in0=P[:].rearrange("p b (n c) -> p b n c",
                                       c=C)[:, :, z0:z0 + L, :],
                    in1=expb[:, :, eo:eo + L, :],
                    op=ALU.mult)
                consumed.update(range(z0, z0 + L))
            ready = [r for r in rem_add if all(
                z in consumed for z in range(r[1], r[1] + r[3]))]
            for r in ready:
                rem_add.remove(r)
            # pair-fuse equal-length runs into one op via a strided pair
            # dim (slice [a, a+2d) -> split (2, d) -> slice [0, L*C)):
            # halves the per-op fixed cost of the add stage
            ready.sort(key=lambda r: (r[3], r[1]))
            nbc = SCHED2_L2[2]["nblk"] if dst is not msgs else 9
            while ready:
                r1 = ready.pop(0)
                mate = None
                for r2 in ready:
                    if (r2[3] == r1[3] and r2[1] >= r1[1] + r1[3]
                            and r2[2] >= r1[2] + r1[3]
                            and 2 * r2[1] - r1[1] <= nbc
                            and 2 * r2[2] - r1[2] <= 9):
                        mate = r2
                        break
                if mate is None:
                    (jj, z0, k0, L) = r1
                    nc.vector.tensor_tensor(
                        out=msgs[:, :, k0 * C:(k0 + L) * C],
                        in0=dst[:, :, z0 * C:(z0 + L) * C],
                        in1=msgs[:, :, k0 * C:(k0 + L) * C],
                        op=ALU.add)
                    continue
                ready.remove(mate)
                (jj, z1, k1, L), (_, z2, k2, _) = r1, mate
                dze, dke = (z2 - z1) * C, (k2 - k1) * C
                mo = msgs[:, :, k1 * C:k1 * C + 2 * dke].rearrange(
                    "p b (two d) -> p b two d", two=2)[:, :, :, 0:L * C]
                nc.vector.tensor_tensor(
                    out=mo,
                    in0=dst[:, :, z1 * C:z1 * C + 2 * dze].rearrange(
                        "p b (two d) -> p b two d",
                        two=2)[:, :, :, 0:L * C],
                    in1=mo,
                    op=ALU.add)
            for (jj, z, k, ratio) in [r for r in rem_stt
                                      if r[1] in consumed]:
                rem_stt.remove((jj, z, k, ratio))
                nc.vector.scalar_tensor_tensor(
                    out=msgs[:, :, k * C:(k + 1) * C],
                    in0=dst[:, :, z * C:(z + 1) * C],
                    scalar=float(ratio),
                    in1=msgs[:, :, k * C:(k + 1) * C],
                    op0=ALU.mult, op1=ALU.add)
        assert not rem_add and not rem_stt, (rem_add, rem_stt)

    def emit_fma(sch, j, zj):
        for (jj, z0, k0, L) in sch["runs_add"]:
            if jj != j:
                continue
            nc.vector.tensor_tensor(
                out=msgs[:, :, k0 * C:(k0 + L) * C],
                in0=zj[:, :, z0 * C:(z0 + L) * C],
                in1=msgs[:, :, k0 * C:(k0 + L) * C],
                op=ALU.add)
        for (jj, z, k, ratio) in sch["runs_stt"]:
            if jj != j:
                continue
            nc.vector.scalar_tensor_tensor(
                out=msgs[:, :, k * C:(k + 1) * C],
                in0=zj[:, :, z * C:(z + 1) * C],
                scalar=float(ratio),
                in1=msgs[:, :, k * C:(k + 1) * C],
                op0=ALU.mult, op1=ALU.add)

    for gi, sch in enumerate(SCHED2_L2):
        nblk = sch["nblk"]
        P = P0 if gi == 0 else P12
        for (p, ig0, ni, z0) in sch["pops"]:
            nc.vector.tensor_tensor(
                out=P[:].rearrange("p b (n c) -> p b n c",
                                   c=C)[:, :, z0:z0 + ni, :],
                in0=xg[:].rearrange("p b (n c) -> p b n c",
                                    c=C)[:, :, ig0:ig0 + ni, :],
                in1=w_sb[:, :, None, p * C:(p + 1) * C].broadcast_to(
                    (128, BP, ni, C)),
                op=ALU.mult)
        if gi > 0:
            # kick GpSimd's share first so it overlaps DVE's other j's
            for (j, qoff, segs) in sch["zjq"]:
                if j in GP_JS:
                    emit_zjq(nc.gpsimd, zjG[j], P, qoff, segs)
        for (j, qoff, segs) in sch["zjq"]:
            if gi > 0 and j in GP_JS:
                continue
            dst = msgs if gi == 0 else zj12
            runs = None if gi == 0 else (
                [r for r in sch["runs_add"] if r[0] == j],
                [r for r in sch["runs_stt"] if r[0] == j])
            emit_zjq(nc.vector, dst, P, qoff, segs, runs)
        if gi > 0:
            for j in GP_JS:
                if any(jj == j for (jj, _, _) in sch["zjq"]):
                    emit_fma(sch, j, zjG[j])


def _emit_tp_l1(nc, ALU, xg, w_sb, shB, msgs, pp):
    """msgs[k] = x * w'_k * sh_k ; w' host-expanded+cg-folded to 9 kg blocks."""
    import concourse.mybir as mybir
    bf16 = mybir.dt.bfloat16
    tmp = pp.tile([128, BP, 9, C], bf16, name="l1tmp", tag="l1tmp")
    nc.vector.tensor_tensor(
        out=tmp[:],
        in0=w_sb[:].rearrange("p b (n c) -> p b n c", c=C),
        in1=shB[:].bitcast(bf16),
        op=ALU.mult)
    nc.vector.tensor_tensor(
        out=msgs[:].rearrange("p b (n c) -> p b n c", c=C),
        in0=tmp[:],
        in1=xg[:, :, None, 0:C].broadcast_to((128, BP, 9, C)),
        op=ALU.mult)


# ---------------- host orchestration ----------------
def _chunked_T(feats_own):
    """[NPC, 576] -> kg-blocked transposed [64, 9*NPC]."""
    out = np.empty((64, 9 * NPC), np.float32)
    for kg in range(9):
        out[:, kg * NPC:(kg + 1) * NPC] = feats_own[:, kg * 64:(kg + 1) * 64].T
    return out


def _unchunk_T(newT):
    """[64, 9*NPC] -> [NPC, 576]."""
    out = np.empty((NPC, 576), np.float32)
    for kg in range(9):
        out[:, kg * 64:(kg + 1) * 64] = newT[:, kg * NPC:(kg + 1) * NPC].T
    return out


_CACHE = {}


def _prep(positions, senders, receivers):
    key = (senders.tobytes(), receivers.tobytes(), positions.tobytes())
    if _CACHE.get("key") == key:
        return _CACHE["val"]
    sh_eff, basis = edge_geometry(positions, senders, receivers)
    owner, local, nodes_of, _ = partition_graph(receivers)
    deg_bin = np.zeros(NCORES * NW, np.int64)
    np.add.at(deg_bin, owner[receivers] * NW + local[receivers] // 128, 1)
    tpw = (int(deg_bin.max()) + 127) // 128
    T = NW * tpw
    assert T % BP == 0
    perm = build_core_edges(receivers, owner, local, tpw)

    valid = perm >= 0
    eg = np.where(valid, perm, 0)
    snd = np.where(valid, senders[eg], 0).astype(np.int16)      # [NC, T*128]
    shp_e = sh_eff[eg] * valid[..., None]                        # [NC, T*128, 9]
    bas_e = basis[eg] * valid[..., None]                         # [NC, T*128, 8]
    lr = np.where(valid, local[receivers[eg]], 0)

    NG = T // BP
    inv = np.float32(1.0 / np.sqrt(AVG_NN))
    sidx = np.empty((NCORES, 128, T * 128 // 16), np.int16)
    shp_h = np.empty((NCORES, 128, T, 9), np.float32)
    shq_h = np.empty((NCORES, 128, T, NSHQ), np.float32)
    jcols = np.array([j for (j, cg) in SHQ_COLS])
    cgv = np.array([cg for (j, cg) in SHQ_COLS], np.float32)
    bas_h = np.empty((NCORES, NG, 8, BP * 128), BF16)
    smat_h = np.zeros((NCORES, 128, T, 128), BF16)
    for k in range(NCORES):
        s = snd[k].reshape(T * 8, 16)
        sidx[k] = np.tile(s.T, (8, 1))
        shp_f = shp_e[k].reshape(T, 128, 9).transpose(1, 0, 2)
        # fp32-packed (v, v) bf16 pair so on-chip broadcast-expansion moves
        # half the elements (bitcast back to bf16 at the consumer)
        up = shp_f.astype(BF16).view(np.uint16).astype(np.uint32)
        shp_h[k] = ((up << 16) | up).view(np.float32)
        u = (shp_f[:, :, jcols] * cgv[None, None, :]).astype(
            BF16).view(np.uint16).astype(np.uint32)
        shq_h[k] = ((u << 16) | u).view(np.float32)
        bas_h[k] = bas_e[k].reshape(NG, BP * 128, 8).transpose(0, 2, 1).astype(BF16)
        v = valid[k]
        e_slots = np.arange(T * 128)
        p_, t_ = e_slots % 128, e_slots // 128
        cols = lr[k] - (t_ // tpw) * 128
        ok = v & (cols >= 0) & (cols < 128)
        sm = np.zeros((128, T, 128), np.float32)
        sm[p_[ok], t_[ok], cols[ok]] = inv
        smat_h[k] = sm.astype(BF16)
    val = dict(T=T, NG=NG, tpw=tpw, nodes_of=nodes_of, sidx=sidx,
               shp_h=shp_h, bas_h=bas_h, smat_h=smat_h, shq_h=shq_h)
    _CACHE["key"], _CACHE["val"] = key, val
    return val


EXEC_NS = []


def _run_layer(nc, pre, ftab, oldT_by_core, lw):
    import os
    from concourse.bass_utils import run_bass_kernel_spmd
    in_maps = []
    for k in range(NCORES):
        m = dict(ftab=ftab,
                 sidx=pre["sidx"][k],
                 shp=pre["shp_h"][k],
                 shq=pre["shq_h"][k],
                 basisT=pre["bas_h"][k],
                 smat=pre["smat_h"][k],
                 oldT=oldT_by_core[k],
                 w1=lw["w1"], b1=lw["b1"], w2=lw["w2"], b2=lw["b2"],
                 w3=lw["w3"], lin0=lw["lin"][0], lin1=lw["lin"][1],
                 lin2=lw["lin"][2], gw0=lw["gw"][0], gw1=lw["gw"][1],
                 gb0=lw["gb"][0], gb1=lw["gb"][1])
        in_maps.append(m)
    trace = bool(os.environ.get("KERNEL_TRACE"))
    res = run_bass_kernel_spmd(nc, in_maps, list(range(NCORES)), trace=trace,
                               trace_cores=list(range(NCORES)) if trace else None)
    if trace and res.exec_time_ns is not None:
        EXEC_NS.append(res.exec_time_ns)
    return [res.results[k]["newT"] for k in range(NCORES)]


def _layer_weights(inputs, i, layer2):
    f32 = np.float32
    w3 = np.ascontiguousarray(inputs["mlp_w3"][i], f32)
    if layer2:
        w3p = w3.astype(BF16)
    else:
        w3p = np.empty((H, 576), BF16)
        for kg in range(9):
            p = L1_PATH_OF_K[kg]
            w3p[:, kg * C:(kg + 1) * C] = (
                w3[:, p * C:(p + 1) * C] * L1_CG_OF_K[kg]).astype(BF16)
    return dict(
        w1=np.ascontiguousarray(inputs["mlp_w1"][i], f32).astype(BF16),
        b1=np.ascontiguousarray(inputs["mlp_b1"][i], f32).reshape(H, 1),
        w2=np.ascontiguousarray(inputs["mlp_w2"][i], f32).astype(BF16),
        b2=np.ascontiguousarray(inputs["mlp_b2"][i], f32).reshape(H, 1),
        w3=w3p,
        lin=[np.ascontiguousarray(inputs["lin_self"][i, l], f32)
             for l in range(3)],
        gw=[np.ascontiguousarray(inputs["gate_w"][i, l], f32)
            for l in range(2)],
        gb=[np.ascontiguousarray(inputs["gate_b"][i, l], f32).reshape(C, 1)
            for l in range(2)],
    )


_KERNEL_CACHE = {}


def _get_kernels(T):
    if T not in _KERNEL_CACHE:
        _KERNEL_CACHE[T] = (build_layer_kernel(False, T),
                            build_layer_kernel(True, T))
    return _KERNEL_CACHE[T]


def _pack_ftab(table, ncols):
    out = np.zeros((N_NODES, ncols), BF16)
    used = min(ncols, table.shape[1])
    out[:, :used] = table[:, :used].astype(BF16)
    return out


def kernel(**inputs):
    positions = np.asarray(inputs["positions"], np.float32)
    species = np.asarray(inputs["species"]).astype(np.int64)
    senders = np.asarray(inputs["senders"]).astype(np.int64)
    receivers = np.asarray(inputs["receivers"]).astype(np.int64)

    pre = _prep(positions, senders, receivers)
    T = pre["T"]
    nc1, nc2 = _get_kernels(T)
    nodes_of = pre["nodes_of"]

    # initial features: x0 from species embedding (host; tiny)
    x0 = (np.asarray(inputs["embed"], np.float32)[species]
          @ np.asarray(inputs["w_proj"], np.float32))          # [N, 64]
    table = np.zeros((N_NODES, F), np.float32)
    table[:, 0:C] = x0

    # ---- layer 1 ----
    oldT = [_chunked_T(table[nodes_of[k]]) for k in range(NCORES)]
    lw = _layer_weights(inputs, 0, False)
    newT = _run_layer(nc1, pre, _pack_ftab(table, GCOLP_L1), oldT, lw)

    table2 = np.empty((N_NODES, F), np.float32)
    for k in range(NCORES):
        table2[nodes_of[k]] = _unchunk_T(newT[k])

    # ---- layer 2 ----
    lw = _layer_weights(inputs, 1, True)
    newT2 = _run_layer(nc2, pre, _pack_ftab(table2, GCOLP_L2), newT, lw)

    table3 = np.empty((N_NODES, F), np.float32)
    for k in range(NCORES):
        table3[nodes_of[k]] = _unchunk_T(newT2[k])

    # ---- output: reorder component-major -> reference layout + alpha ----
    t3 = table3.reshape(N_NODES, 9, C)
    out = np.empty((N_NODES, F), np.float32)
    out[:, 0:64] = t3[:, 0]
    out[:, 64:256] = (0.5 * t3[:, 1:4]).transpose(0, 2, 1).reshape(N_NODES, 192)
    out[:, 256:576] = (0.25 * t3[:, 4:9]).transpose(0, 2, 1).reshape(N_NODES, 320)
    return out

